# revision 9
# baseline (speedup 1.0000x reference)
"""FAVOR+ (Performer) non-causal linear attention on 8 Trainium2 NeuronCores.

Sharding: data-parallel over batch B=8 -> one batch element per core.

Per-core pipeline (L=4096, DIM=768, H=12, D=64, M=256):
  prep : cast-DMA weights to fp16, PE-transpose to feature-major, split into
         fp8e4m3 hi/lo pairs (scaled) for DoubleRow matmuls; DMA blocks
         interleaved with chunk-0/1 transposes and v so PE never starves
  pass1: per 512-row chunk: cast-DMA x to fp16; PE-transpose -> xT hi/lo fp8;
         kT/qT/v via fp8 DoubleRow hi/lo-compensated GEMMs (4.5 cyc per
         128x512 output tile instead of 6); k features fp16 with blockdiag pm
         (2 heads per matmul); kv accumulated m-major [m,65] into 4 persistent
         PSUM banks across all chunks (ones-augmented v gives k_sum for
         free); qT staged in SBUF fp16 (no DRAM round trip).  Emission is
         software-pipelined: transposes/v run 2 chunks ahead; kv trails one
         pair behind its kp conversion.
  mid  : kv PSUM -> fp16 SBUF (already m-major); eps*colsum(kv) rows for the
         ACT-assigned q-feature heads
  pass2: q features m-major fp16 (computed one chunk ahead, spread across the
         l-tile loop; relu+eps on DVE/Pool, plain relu on ACT with the eps
         restored by a rank-1 matmul into num); num L-major [l,65] (den =
         col 64); reciprocal + broadcast multiply on DVE; attn PE-transposed
         to feature-major, split fp8 hi/lo; y via DoubleRow GEMM -> DMA out
"""

import math
import os
import sys
from contextlib import ExitStack

import numpy as np

for _p in ("/opt/trn_rl_repo",):
    if _p not in sys.path and os.path.isdir(_p):
        sys.path.insert(0, _p)

import concourse.bass as bass  # noqa: E402
import concourse.mybir as mybir  # noqa: E402
import concourse.tile as tile  # noqa: E402
from concourse import bacc  # noqa: E402

P = 128
DIM = 768
H = 12
D = 64
M = 256
KT = DIM // P   # 6 contraction c-tiles
NPAIR = H // 2  # 6 head pairs
EPS = 1e-3
RATIO = 1.0 / math.sqrt(float(M))

SX = 16.0    # x ~ N(0,1)
SW = 32.0    # qkv_w ~ N(0, 1/768)
SA = 64.0    # attn ~ 0.1
SP = 32.0    # proj_w ~ N(0, 1/768)
SKT = 16.0   # kT ~ N(0,1) -> fp8 for the k-feature GEMM
SPM = 32.0   # RATIO*pm ~ N(0, 1/256) -> fp8
SKP = SKT * SPM  # k-feature path runs in this scaled domain until kvm

F32 = mybir.dt.float32
F16 = mybir.dt.float16
F8 = mybir.dt.float8e4
AL = mybir.AluOpType
AF = mybir.ActivationFunctionType
DR = mybir.MatmulPerfMode.DoubleRow

# pass-2 q-feature conversion engine per (head, mtile) slot ai=h*2+mt.
# Slots of the group-starting heads (ai 0,1,12,13) stay off ACT so each num
# PSUM group's first instruction is its start matmul.
_QP_ENG = {}
_c = 0
for _ai in range(2 * H):
    if _ai in (0, 1, 12, 13):
        _QP_ENG[_ai] = "dve"
    else:
        _QP_ENG[_ai] = ("act", "act", "dve")[_c % 3]
        _c += 1


def build(L=4096, has_qkv_b=False, has_proj_b=False):
    LCH = 512
    NCH = L // LCH
    NSUB = LCH // P  # 4

    nc = bacc.Bacc("TRN2", target_bir_lowering=False, debug=False)
    x_d = nc.dram_tensor("x", [L, DIM], F32, kind="ExternalInput").ap()
    qkvw_d = nc.dram_tensor("qkv_w", [3 * DIM, DIM], F32, kind="ExternalInput").ap()
    qkvb_d = nc.dram_tensor("qkv_b", [3 * DIM], F32, kind="ExternalInput").ap()
    projw_d = nc.dram_tensor("proj_w", [DIM, DIM], F32, kind="ExternalInput").ap()
    projb_d = nc.dram_tensor("proj_b", [DIM], F32, kind="ExternalInput").ap()
    pm_d = nc.dram_tensor("proj_mat", [M, D], F32, kind="ExternalInput").ap()
    y_d = nc.dram_tensor("y", [L, DIM], F32, kind="ExternalOutput").ap()

    with tile.TileContext(nc) as tc:
        with ExitStack() as ctx:
            _body(ctx, tc, x_d, qkvw_d, qkvb_d, projw_d, projb_d, pm_d, y_d,
                  L, LCH, NCH, NSUB, has_qkv_b, has_proj_b)
    nc.compile()
    return nc


def _dr_gemm(nc, out, whl, xhl, wcols, lt=None, bias=None):
    """Accumulating fp8 DoubleRow hi/lo-compensated GEMM over 768 contraction.

    whl/xhl: [128, KT, 2, *] fp8 with hi at [:,kk,0,:], lo at [:,kk,1,:].
    lt=None : out[wc, l]; stationary = whl cols wcols, moving = xhl  (kT/qT)
    lt given: out[l, wc]; stationary = xhl l-tile cols, moving = whl (v/y)
    """
    c0, c1 = wcols
    n = c1 - c0
    if lt is None:
        for i in range(KT // 2):
            for kk in (2 * i, 2 * i + 1):
                nc.tensor.matmul(
                    out, whl[:, kk, :, c0:c1],
                    xhl[:, kk, 0:1, :].to_broadcast([P, 2, out.shape[-1]]),
                    start=(kk == 0), stop=False, perf_mode=DR,
                )
            nc.tensor.matmul(
                out, whl[:, 2 * i : 2 * i + 2, 0, c0:c1],
                xhl[:, 2 * i : 2 * i + 2, 1, :],
                start=False, stop=(bias is None and i == KT // 2 - 1),
                perf_mode=DR,
            )
    else:
        l0 = lt * P
        for i in range(KT // 2):
            for kk in (2 * i, 2 * i + 1):
                nc.tensor.matmul(
                    out, xhl[:, kk, :, l0 : l0 + P],
                    whl[:, kk, 0:1, c0:c1].to_broadcast([P, 2, n]),
                    start=(kk == 0), stop=False, perf_mode=DR,
                )
            nc.tensor.matmul(
                out, xhl[:, 2 * i : 2 * i + 2, 0, l0 : l0 + P],
                whl[:, 2 * i : 2 * i + 2, 1, c0:c1],
                start=False, stop=(bias is None and i == KT // 2 - 1),
                perf_mode=DR,
            )
    if bias is not None:
        ones_row, brow = bias
        nc.tensor.matmul(out, ones_row, brow, start=False, stop=True)


def _body(ctx, tc, x_d, qkvw_d, qkvb_d, projw_d, projb_d, pm_d, y_d,
          L, LCH, NCH, NSUB, has_qkv_b, has_proj_b):
    nc = tc.nc
    iqkv = 1.0 / (SX * SW)
    iy = 1.0 / (SA * SP)

    persist = ctx.enter_context(tc.tile_pool(name="persist", bufs=1))

    ident16 = persist.tile([P, P], F16, tag="ident16", name="ident16")[:]
    nc.gpsimd.memset(ident16, 0.0)
    nc.gpsimd.affine_select(
        out=ident16, in_=ident16, compare_op=AL.not_equal, fill=1.0,
        base=0, pattern=[[-1, P]], channel_multiplier=1,
    )
    ones1 = persist.tile([1, P], F16, tag="ones1", name="ones1")[:]
    nc.gpsimd.memset(ones1, 1.0)
    epscol = persist.tile([P, 1], F16, tag="epscol", name="epscol")[:]
    nc.gpsimd.memset(epscol, EPS)

    whl_qk = persist.tile([P, KT, 2, 2 * DIM], F8, tag="whl_qk", name="whl_qk")[:]
    wvhl = persist.tile([P, KT, 2, DIM], F8, tag="wvhl", name="wvhl")[:]
    wphl = persist.tile([P, KT, 2, DIM], F8, tag="wphl", name="wphl")[:]
    # folded blockdiag pm for the fp8 DoubleRow k-feature GEMM:
    # slice 0 = [RATIO*pmT | 0] (c rows 0:64), slice 1 = [0 | RATIO*pmT]
    pmbd = persist.tile([P, 2, 2 * M], F8, tag="pmbd", name="pmbd")[:]
    pmt2 = persist.tile([P, M], F16, tag="pmt2", name="pmt2")[:]
    qt_sb = persist.tile([P, NPAIR, L], F16, tag="qt_sb", name="qt_sb")[:]
    kvm = persist.tile([P, 4, 6, D + 1], F16, tag="kvm", name="kvm")[:]
    kvmcs = persist.tile([1, 4, 6, D + 1], F16, tag="kvmcs", name="kvmcs")[:]

    if has_qkv_b:
        qkb = persist.tile([P, 2 * KT], F32, tag="qkb", name="qkb")[:]
        nc.sync.dma_start(qkb, qkvb_d.rearrange("(t p) -> p t", p=P)[:, 0 : 2 * KT])
        qkbk = persist.tile([P, KT], F32, tag="qkbk", name="qkbk")[:]
        nc.scalar.activation(qkbk, qkb[:, KT : 2 * KT], AF.Copy, scale=SKT)
        vbf = persist.tile([1, DIM], F32, tag="vbf", name="vbf")[:]
        nc.sync.dma_start(vbf, qkvb_d[2 * DIM : 3 * DIM].unsqueeze(0))
        vb_row = persist.tile([1, DIM], F16, tag="vb_row", name="vb_row")[:]
        nc.scalar.activation(vb_row, vbf, AF.Copy, scale=SX * SW)
    if has_proj_b:
        pbf = persist.tile([1, DIM], F32, tag="pbf", name="pbf")[:]
        nc.sync.dma_start(pbf, projb_d.unsqueeze(0))
        pb_row = persist.tile([1, DIM], F16, tag="pb_row", name="pb_row")[:]
        nc.scalar.activation(pb_row, pbf, AF.Copy, scale=SA * SP)

    vsb = persist.tile([P, 2, NSUB, H, D + 1], F16, tag="vsb", name="vsb")[:]
    nc.gpsimd.memset(vsb[:, :, :, :, D : D + 1], 1.0)

    # kv accumulator in SBUF fp32 (indexed by ai = h*2+mt)
    kv_acc = persist.tile([P, 2 * H, D + 1], F32, tag="kv_acc", name="kv_acc")[:]

    # pass-2 q-feature tiles, double-buffered by chunk parity
    qp2 = [persist.tile([P, H, 2, LCH], F16, tag=f"qp2_{i}", name=f"qp2_{i}")[:]
           for i in range(2)]

    def emit_qp(ich, heads, psum_pool, tag):
        l0 = ich * LCH
        qp_sb = qp2[ich % 2]
        for h in heads:
            p, h2 = h // 2, h % 2
            for mt in range(2):
                qps = psum_pool.tile([P, LCH], F32, tag=tag, name="qps")[:]
                nc.tensor.matmul(
                    qps,
                    pmt2[h2 * D : (h2 + 1) * D, mt * P : (mt + 1) * P],
                    qt_sb[h2 * D : (h2 + 1) * D, p, l0 : l0 + LCH],
                    start=True, stop=True)
                eng = _QP_ENG[h * 2 + mt]
                if eng == "act":
                    # plain relu; eps restored via rank-1 kvmcs in num
                    nc.scalar.activation(qp_sb[:, h, mt, :], qps, AF.Relu)
                else:
                    nc.vector.tensor_scalar(qp_sb[:, h, mt, :], qps,
                                            EPS, EPS, AL.add, AL.max)

    if True:
        with tc.tile_pool(name="p1x", bufs=2) as xp, \
             tc.tile_pool(name="p1xhl", bufs=2) as xhlp, \
             tc.tile_pool(name="p1kt", bufs=3) as ktp, \
             tc.tile_pool(name="p1kp", bufs=8) as kpp, \
             tc.tile_pool(name="wprep", bufs=3) as wpool, \
             tc.tile_pool(name="p1work", bufs=8, space="PSUM") as work:

            xnats = {}
            xhls = {}

            def dma_x(ich):
                l0 = ich * LCH
                xnat = xp.tile([P, NSUB, DIM], F16, tag="xnat", name="xnat")[:]
                nc.gpsimd.dma_start(
                    xnat,
                    x_d[l0 : l0 + LCH, :].rearrange("(s p) k -> p s k", p=P))
                xnats[ich] = xnat

            def transp_x(ich):
                xnat = xnats.pop(ich)
                xhl = xhlp.tile([P, KT, 2, LCH], F8, tag="xhl", name="xhl")[:]
                for kk in range(KT):
                    tp = work.tile([P, 512], F16, tag="work", name="ttp")[:]
                    for s in range(NSUB):
                        nc.tensor.transpose(
                            tp[:, s * P : (s + 1) * P],
                            xnat[:, s, kk * P : (kk + 1) * P], ident16)
                    nc.scalar.activation(xhl[:, kk, 0, :], tp, AF.Copy,
                                         scale=SX)
                    nc.vector.scalar_tensor_tensor(
                        xhl[:, kk, 1, :], tp, SX, xhl[:, kk, 0, :],
                        AL.mult, AL.subtract)
                xhls[ich] = xhl

            def emit_v(ich, subs):
                vs = ich % 2
                xhl = xhls[ich]
                for s in subs:
                    for ci in range(2):
                        c0 = ci * 384
                        vps = work.tile([P, 512], F32, tag="work", name="vps")[:]
                        bias = None
                        if has_qkv_b:
                            bias = (ones1, vb_row[:, c0 : c0 + 384])
                        _dr_gemm(nc, vps[:, 0:384], wvhl, xhl, (c0, c0 + 384),
                                 lt=s, bias=bias)
                        nc.scalar.activation(
                            vsb[:, vs, s, 6 * ci : 6 * ci + 6, 0:D],
                            vps[:, 0:384].rearrange("p (h d) -> p h d", d=D),
                            AF.Copy, scale=iqkv)

            def emit_kT(ich, p):
                ktps = work.tile([P, 512], F32, tag="work", name="ktps")[:]
                _dr_gemm(nc, ktps, whl_qk, xhls[ich],
                         (DIM + p * P, DIM + (p + 1) * P))
                # fold [128,512] -> [64, 2, 512] fp8 (scaled) for DoubleRow
                kt = ktp.tile([P, 2, LCH], F8, tag="kt", name="kt")[:]
                for h2 in range(2):
                    if has_qkv_b:
                        nc.scalar.activation(
                            kt[0:D, h2, :], ktps[h2 * D : (h2 + 1) * D, :],
                            AF.Identity,
                            bias=qkbk[h2 * D : (h2 + 1) * D, p : p + 1],
                            scale=SKT * iqkv)
                    else:
                        nc.scalar.activation(
                            kt[0:D, h2, :], ktps[h2 * D : (h2 + 1) * D, :],
                            AF.Copy, scale=SKT * iqkv)
                return kt

            def emit_kp(kt):
                kps = []
                for lt in range(NSUB):
                    kpps = work.tile([P, 512], F32, tag="work", name="kpps")[:]
                    nc.tensor.matmul(kpps,
                                     kt[0:D, :, lt * P : (lt + 1) * P],
                                     pmbd[0:D], start=True, stop=True,
                                     perf_mode=DR)
                    kp = kpp.tile([P, 2 * M], F16, tag="kp", name="kp")[:]
                    # k-feature path is scaled by SKP; it cancels in num/den
                    nc.vector.tensor_scalar(kp, kpps, SKP * EPS, SKP * EPS,
                                            AL.add, AL.max)
                    kps.append(kp)
                return kps

            def emit_qT(ich, p):
                l0 = ich * LCH
                qtps = work.tile([P, 512], F32, tag="work", name="qtps")[:]
                _dr_gemm(nc, qtps, whl_qk, xhls[ich], (p * P, (p + 1) * P))
                if has_qkv_b:
                    nc.scalar.activation(qt_sb[:, p, l0 : l0 + LCH], qtps,
                                         AF.Identity,
                                         bias=qkb[:, p : p + 1], scale=iqkv)
                else:
                    nc.scalar.activation(qt_sb[:, p, l0 : l0 + LCH], qtps,
                                         AF.Copy, scale=iqkv)

            def emit_kv(ich, p, kps):
                vs = ich % 2
                kvp = work.tile([P, 4, D + 1], F32, tag="work", name="kvp")[:]
                for lt in range(NSUB):
                    kp = kps[lt]
                    for h2 in range(2):
                        h = 2 * p + h2
                        for mt in range(2):
                            j = h2 * 2 + mt
                            nc.tensor.matmul(
                                kvp[:, j, :],
                                kp[:, j * P : (j + 1) * P],
                                vsb[:, vs, lt, h, :],
                                start=(lt == 0 and j == 0),
                                stop=(lt == NSUB - 1 and j == 3),
                            )
                nc.vector.tensor_tensor(
                    kv_acc[:, 4 * p : 4 * p + 4, :], kvp,
                    kv_acc[:, 4 * p : 4 * p + 4, :], AL.add)

            # ---- prep: weight DMA blocks interleaved with chunk-0/1 work ----
            def prep_w_blocks(src, nrows, dst, dst_off, scale):
                blocks = []
                nt = nrows // P
                c0 = 0
                while c0 < nt:
                    bs = min(3, nt - c0)
                    st = {}

                    def bdma(c0=c0, bs=bs, st=st):
                        wnat = wpool.tile([P, 3, DIM], F16, tag="wnat",
                                          name="wnat")[:]
                        nc.gpsimd.dma_start(
                            wnat[:, 0:bs, :],
                            src[c0 * P : (c0 + bs) * P, :].rearrange(
                                "(s p) k -> p s k", p=P))
                        st["wnat"] = wnat

                    def bcomp(c0=c0, bs=bs, st=st):
                        wnat = st["wnat"]
                        for kk in range(KT):
                            tp = work.tile([P, 512], F16, tag="work",
                                           name="ptp")[:]
                            for j in range(bs):
                                nc.tensor.transpose(
                                    tp[:, j * P : (j + 1) * P],
                                    wnat[:, j, kk * P : (kk + 1) * P], ident16)
                            hi = dst[:, kk, 0,
                                     dst_off + c0 * P : dst_off + (c0 + bs) * P]
                            nc.scalar.activation(hi, tp[:, 0 : bs * P], AF.Copy,
                                                 scale=scale)
                            nc.vector.scalar_tensor_tensor(
                                dst[:, kk, 1,
                                    dst_off + c0 * P : dst_off + (c0 + bs) * P],
                                tp[:, 0 : bs * P], scale, hi,
                                AL.mult, AL.subtract)

                    blocks.append((bdma, bcomp))
                    c0 += bs
                return blocks

            pm_st = {}

            def prep_pm_dma():
                pmn = wpool.tile([P, 2, D], F16, tag="pmn", name="pmn")[:]
                nc.gpsimd.dma_start(pmn, pm_d.rearrange("(s p) d -> p s d", p=P))
                pm_st["pmn"] = pmn

            def prep_pm():
                pmn = pm_st["pmn"]
                tp = work.tile([P, 512], F16, tag="work", name="ptp")[:]
                for s in range(2):
                    nc.tensor.transpose(tp[0:D, s * P : (s + 1) * P],
                                        pmn[:, s, :], ident16)
                nc.gpsimd.memset(pmbd, 0.0)
                nc.scalar.activation(pmbd[0:D, 0, 0:M], tp[0:D, 0:M], AF.Copy,
                                     scale=SPM * RATIO)
                nc.scalar.activation(pmbd[0:D, 1, M : 2 * M], tp[0:D, 0:M],
                                     AF.Copy, scale=SPM * RATIO)
                nc.scalar.activation(pmt2[0:D, :], tp[0:D, 0:M], AF.Copy,
                                     scale=RATIO)
                nc.scalar.activation(pmt2[D:P, :], tp[0:D, 0:M], AF.Copy,
                                     scale=RATIO)

            dma_x(0)
            wv = prep_w_blocks(qkvw_d[2 * DIM : 3 * DIM, :], DIM, wvhl, 0, SW)
            wqk_k = prep_w_blocks(qkvw_d[DIM : 2 * DIM, :], DIM, whl_qk,
                                  DIM, SW)
            wqk_q = prep_w_blocks(qkvw_d[0:DIM, :], DIM, whl_qk, 0, SW)
            wp = prep_w_blocks(projw_d, DIM, wphl, 0, SP)

            nc.gpsimd.memset(kv_acc, 0.0)
            blocks = wv + wqk_k + wqk_q + wp
            bst = {"dma": 0, "comp": 0}

            def bdma_next():
                if bst["dma"] < len(blocks):
                    blocks[bst["dma"]][0]()
                    bst["dma"] += 1

            def bcomp_next():
                if bst["comp"] < len(blocks):
                    blocks[bst["comp"]][1]()
                    bst["comp"] += 1
                    bdma_next()

            nop = lambda: None
            # phase A: wv + k-part of wqk; q-part and proj stream into chunk 0
            nA = len(wv) + len(wqk_k)
            fillers = {
                0: [lambda: transp_x(0),
                    (lambda: dma_x(1)) if 1 < NCH else nop],
                1: [prep_pm],
                2: [lambda: emit_v(0, (0,)), lambda: emit_v(0, (1,))],
                3: [lambda: emit_v(0, (2,)), lambda: emit_v(0, (3,)),
                    (lambda: transp_x(1)) if 1 < NCH else nop,
                    (lambda: dma_x(2)) if 2 < NCH else nop],
            }
            bdma_next()
            prep_pm_dma()
            bdma_next()
            # PE p-state warmup: burn the cold DMA-wait ramping the clock
            wu = work.tile([P, 512], F16, tag="work", name="wu")[:]
            for _ in range(7):
                for s in range(4):
                    nc.tensor.transpose(wu[:, s * P : (s + 1) * P], ident16,
                                        ident16)
            for i in range(nA):
                for f in fillers.get(i, []):
                    f()
                bcomp_next()

            # ---- pass 1 main loop ----
            for ich in range(NCH):
                first = ich == 0
                pend_kt = None
                pend = None
                for p in range(NPAIR):
                    kt = emit_kT(ich, p)
                    if first:
                        # stream remaining weight blocks (q-part + proj)
                        bcomp_next()
                        if p >= 3:
                            emit_qT(0, p - 3)
                    else:
                        emit_qT(ich, p)
                    if pend is not None:
                        emit_kv(ich, pend[0], pend[1])
                        if ich == NCH - 1:
                            pp = pend[0]
                            nc.scalar.activation(
                                kvm.rearrange("p b j c -> p (b j) c")[
                                    :, 4 * pp : 4 * pp + 4, :],
                                kv_acc[:, 4 * pp : 4 * pp + 4, :],
                                AF.Copy, scale=1.0 / SKP)
                        pend = None
                    if pend_kt is not None:
                        pend = (pend_kt[0], emit_kp(pend_kt[1]))
                    pend_kt = (p, kt)
                    if ich == NCH - 1 and not first:
                        # chunk-0 q features computed here so pass 2 starts hot
                        emit_qp(0, (2 * p, 2 * p + 1), work, "work")
                    if p == 0 and 1 <= ich and ich + 1 < NCH:
                        transp_x(ich + 1)
                    if p == 3 and 1 <= ich and ich + 2 < NCH:
                        dma_x(ich + 2)
                    if p >= 2 and ich + 1 < NCH:
                        emit_v(ich + 1, (p - 2,))
                if pend is not None:
                    emit_kv(ich, pend[0], pend[1])
                    if ich == NCH - 1:
                        pp = pend[0]
                        nc.scalar.activation(
                            kvm.rearrange("p b j c -> p (b j) c")[
                                :, 4 * pp : 4 * pp + 4, :],
                            kv_acc[:, 4 * pp : 4 * pp + 4, :],
                            AF.Copy, scale=1.0 / SKP)
                pend = (pend_kt[0], emit_kp(pend_kt[1]))
                emit_kv(ich, pend[0], pend[1])
                if ich == NCH - 1:
                    nc.scalar.activation(
                        kvm.rearrange("p b j c -> p (b j) c")[:, 20:24, :],
                        kv_acc[:, 20:24, :], AF.Copy, scale=1.0 / SKP)
                if first:
                    for p3 in range(3, NPAIR):
                        emit_qT(0, p3)
                    if NCH == 1:
                        emit_qp(0, range(H), work, "work")
                xhls.pop(ich)

    with tc.tile_pool(name="csps", bufs=1, space="PSUM") as cspool:
        css = []
        for b in range(4):
            cs = cspool.tile([1, 6, D + 1], F32, tag=f"cs{b}", name="cs")[:]
            for j in range(6):
                nc.tensor.matmul(cs[:, j, :], epscol, kvm[:, b, j, :],
                                 start=(j == 0), stop=(j == 5))
            css.append(cs)
        for b in range(4):
            nc.scalar.copy(kvmcs[:, b], css[b])

    # ---- pass 2 ----
    with tc.tile_pool(name="p2attn", bufs=2) as atp_sb, \
         tc.tile_pool(name="p2rd", bufs=2) as rdp, \
         tc.tile_pool(name="p2ahl", bufs=2) as ahlp, \
         tc.tile_pool(name="p2y", bufs=2) as yp, \
         tc.tile_pool(name="ps2qp", bufs=3, space="PSUM") as qppsum, \
         tc.tile_pool(name="ps2nm", bufs=2, space="PSUM") as numpsum, \
         tc.tile_pool(name="ps2at", bufs=1, space="PSUM") as atpsum, \
         tc.tile_pool(name="ps2y", bufs=1, space="PSUM") as ypsum:

        def emit_y(ich, ahl, lt):
            l0 = ich * LCH
            yps = ypsum.tile([P, DIM], F32, tag="yps", name="yps")[:]
            for c0, c1 in ((0, 512), (512, DIM)):
                b = (ones1, pb_row[:, c0:c1]) if has_proj_b else None
                _dr_gemm(nc, yps[:, c0:c1], wphl, ahl, (c0, c1), lt=lt, bias=b)
            ysb = yp.tile([P, DIM], F32, tag="ysb", name="ysb")[:]
            nc.scalar.activation(ysb, yps, AF.Copy, scale=iy)
            nc.sync.dma_start(y_d[l0 + lt * P : l0 + (lt + 1) * P, :], ysb)

        pend_y = []
        for ich in range(NCH):
            qp_sb = qp2[ich % 2]
            ahl = ahlp.tile([P, KT, 2, LCH], F8, tag="ahl", name="ahl")[:]
            for lt in range(NSUB):
                nmps = [numpsum.tile([P, 6, D + 1], F32, tag="nm", name="nmps")[:]
                        for _ in range(2)]
                for h in range(H):
                    g = h // 6
                    for mt in range(2):
                        ai = h * 2 + mt
                        if _QP_ENG[ai] == "act" and h % 6 != 0:
                            nc.tensor.matmul(
                                nmps[g][:, h % 6, :], ones1,
                                kvmcs[0:1, ai // 6, ai % 6, :],
                                start=False, stop=False)
                    for mt in range(2):
                        ai = h * 2 + mt
                        nc.tensor.matmul(
                            nmps[g][:, h % 6, :],
                            qp_sb[:, h, mt, lt * P : (lt + 1) * P],
                            kvm[:, ai // 6, ai % 6, :],
                            start=(mt == 0 and h % 6 == 0),
                            stop=(mt == 1 and h % 6 == 5))
                rd = rdp.tile([P, H], F32, tag="rd", name="rd")[:]
                attn = atp_sb.tile([P, H, D], F16, tag="attn", name="attn")[:]
                for g in range(2):
                    nc.vector.reciprocal(rd[:, g * 6 : (g + 1) * 6],
                                         nmps[g][:, :, D])
                    nc.vector.tensor_tensor(
                        attn[:, g * 6 : (g + 1) * 6, :],
                        nmps[g][:, :, 0:D],
                        rd[:, g * 6 : (g + 1) * 6, None].to_broadcast([P, 6, D]),
                        AL.mult)
                if ich + 1 < NCH:
                    emit_qp(ich + 1, range(3 * lt, 3 * lt + 3), qppsum, "qps")
                if len(pend_y) >= (2 if ich + 1 < NCH else 1):
                    emit_y(*pend_y.pop(0))
                atps = atpsum.tile([P, DIM], F16, tag="at", name="atps")[:]
                for kk in range(KT):
                    nc.tensor.transpose(atps[:, kk * P : (kk + 1) * P],
                                        attn[:, 2 * kk : 2 * kk + 2, :], ident16)
                a3 = atps.rearrange("p (k l) -> p k l", k=KT)
                nc.scalar.activation(ahl[:, :, 0, lt * P : (lt + 1) * P], a3,
                                     AF.Copy, scale=SA)
                nc.vector.scalar_tensor_tensor(
                    ahl[:, :, 1, lt * P : (lt + 1) * P], a3, SA,
                    ahl[:, :, 0, lt * P : (lt + 1) * P], AL.mult, AL.subtract)
                pend_y.append((ich, ahl, lt))
        for args in pend_y:
            emit_y(*args)


_CACHE = {}


def _get_nc(L=4096, hqb=False, hpb=False):
    key = ("nc", L, hqb, hpb)
    if key not in _CACHE:
        _CACHE[key] = build(L, hqb, hpb)
    return _CACHE[key]


last_exec_time_ns = None
last_profile = None


def kernel(x, qkv_w, qkv_b, proj_w, proj_b, proj_mat):
    global last_exec_time_ns, last_profile
    from concourse.bass_utils import run_bass_kernel_spmd

    x = np.asarray(x, np.float32)
    B, L, _ = x.shape
    hqb = bool(np.any(np.asarray(qkv_b)))
    hpb = bool(np.any(np.asarray(proj_b)))
    nc = _get_nc(L, hqb, hpb)
    base = {
        "qkv_w": np.ascontiguousarray(np.asarray(qkv_w, np.float32)),
        "qkv_b": np.ascontiguousarray(np.asarray(qkv_b, np.float32)),
        "proj_w": np.ascontiguousarray(np.asarray(proj_w, np.float32)),
        "proj_b": np.ascontiguousarray(np.asarray(proj_b, np.float32)),
        "proj_mat": np.ascontiguousarray(np.asarray(proj_mat, np.float32)),
    }
    in_maps = [dict(base, x=np.ascontiguousarray(x[b])) for b in range(B)]
    trace = bool(int(os.environ.get("KERNEL_TRACE", "0")))
    res = run_bass_kernel_spmd(nc, in_maps, core_ids=list(range(B)), trace=trace)
    last_exec_time_ns = res.exec_time_ns
    last_profile = res.profile_json
    return np.stack([res.results[b]["y"] for b in range(B)], axis=0)


def _ref_np(x, qkv_w, qkv_b, proj_w, proj_b, proj_mat):
    Ls = x.shape[0]
    qkv = x @ qkv_w.T + qkv_b
    qkv = qkv.reshape(Ls, 3, H, D)
    q, k, v = qkv[:, 0], qkv[:, 1], qkv[:, 2]
    qp = np.maximum(RATIO * np.einsum("lhd,md->lhm", q, proj_mat), 0) + EPS
    kp = np.maximum(RATIO * np.einsum("lhd,md->lhm", k, proj_mat), 0) + EPS
    kv = np.einsum("lhm,lhd->hmd", kp, v)
    ks = kp.sum(axis=0)
    num = np.einsum("lhm,hmd->lhd", qp, kv)
    den = np.einsum("lhm,hm->lh", qp, ks)
    out = (num / den[..., None]).reshape(Ls, DIM)
    return out @ proj_w.T + proj_b


if __name__ == "__main__":
    from concourse.bass_interp import CoreSim

    Ls = int(os.environ.get("SIM_L", "512"))
    use_bias = bool(int(os.environ.get("SIM_BIAS", "1")))
    rng = np.random.default_rng(0)
    x = rng.standard_normal((Ls, DIM), dtype=np.float32)
    qkv_w = (rng.standard_normal((3 * DIM, DIM), dtype=np.float32) * DIM**-0.5)
    qkv_b = (rng.standard_normal(3 * DIM, dtype=np.float32) * 0.1
             if use_bias else np.zeros(3 * DIM, np.float32))
    proj_w = (rng.standard_normal((DIM, DIM), dtype=np.float32) * DIM**-0.5)
    proj_b = (rng.standard_normal(DIM, dtype=np.float32) * 0.1
              if use_bias else np.zeros(DIM, np.float32))
    proj_mat = rng.standard_normal((M, D), dtype=np.float32)

    print(f"building L={Ls} bias={use_bias} ...")
    nc = build(Ls, use_bias, use_bias)
    print("simulating ...")
    sim = CoreSim(nc)
    for name, arr in [("x", x), ("qkv_w", qkv_w), ("qkv_b", qkv_b),
                      ("proj_w", proj_w), ("proj_b", proj_b),
                      ("proj_mat", proj_mat)]:
        sim.tensor(name)[:] = arr
    sim.simulate(check_with_hw=False)
    got = np.array(sim.tensor("y"))
    want = _ref_np(x, qkv_w, qkv_b, proj_w, proj_b, proj_mat)
    err = np.abs(got - want)
    rel = np.linalg.norm(got - want) / np.linalg.norm(want)
    print("max abs err:", err.max(), " rel fro err:", rel)
    assert rel < 2e-2, "sim mismatch"
    print("SIM OK")


# revision 11
# speedup vs baseline: 1.0223x; 1.0223x over previous
"""FAVOR+ (Performer) non-causal linear attention on 8 Trainium2 NeuronCores.

Sharding: data-parallel over batch B=8 -> one batch element per core.

Per-core pipeline (L=4096, DIM=768, H=12, D=64, M=256):
  prep : cast-DMA weights to fp16, PE-transpose to feature-major, split into
         fp8e4m3 hi/lo pairs (scaled) for DoubleRow matmuls; DMA blocks
         interleaved with chunk-0/1 transposes and v so PE never starves
  pass1: per 512-row chunk: cast-DMA x to fp16; PE-transpose -> xT hi/lo fp8;
         kT/qT/v via fp8 DoubleRow hi/lo-compensated GEMMs (4.5 cyc per
         128x512 output tile instead of 6); k features fp16 with blockdiag pm
         (2 heads per matmul); kv accumulated m-major [m,65] into 4 persistent
         PSUM banks across all chunks (ones-augmented v gives k_sum for
         free); qT staged in SBUF fp16 (no DRAM round trip).  Emission is
         software-pipelined: transposes/v run 2 chunks ahead; kv trails one
         pair behind its kp conversion.
  mid  : kv PSUM -> fp16 SBUF (already m-major); eps*colsum(kv) rows for the
         ACT-assigned q-feature heads
  pass2: q features m-major fp16 (computed one chunk ahead, spread across the
         l-tile loop; relu+eps on DVE/Pool, plain relu on ACT with the eps
         restored by a rank-1 matmul into num); num L-major [l,65] (den =
         col 64); reciprocal + broadcast multiply on DVE; attn PE-transposed
         to feature-major, split fp8 hi/lo; y via DoubleRow GEMM -> DMA out
"""

import math
import os
import sys
from contextlib import ExitStack

import numpy as np

for _p in ("/opt/trn_rl_repo",):
    if _p not in sys.path and os.path.isdir(_p):
        sys.path.insert(0, _p)

import concourse.bass as bass  # noqa: E402
import concourse.mybir as mybir  # noqa: E402
import concourse.tile as tile  # noqa: E402
from concourse import bacc  # noqa: E402

P = 128
DIM = 768
H = 12
D = 64
M = 256
KT = DIM // P   # 6 contraction c-tiles
NPAIR = H // 2  # 6 head pairs
EPS = 1e-3
RATIO = 1.0 / math.sqrt(float(M))

SX = 16.0    # x ~ N(0,1)
SW = 32.0    # qkv_w ~ N(0, 1/768)
SA = 64.0    # attn ~ 0.1
SP = 32.0    # proj_w ~ N(0, 1/768)
SKT = 16.0   # kT ~ N(0,1) -> fp8 for the k-feature GEMM
SPM = 32.0   # RATIO*pm ~ N(0, 1/256) -> fp8
SKP = SKT * SPM  # k-feature path runs in this scaled domain until kvm

F32 = mybir.dt.float32
F16 = mybir.dt.float16
F8 = mybir.dt.float8e4
AL = mybir.AluOpType
AF = mybir.ActivationFunctionType
DR = mybir.MatmulPerfMode.DoubleRow

# pass-2 q-feature conversion engine per (head, mtile) slot ai=h*2+mt.
# Slots of the group-starting heads (ai 0,1,12,13) stay off ACT so each num
# PSUM group's first instruction is its start matmul.
_QP_ENG = {}
_c = 0
for _ai in range(2 * H):
    if _ai in (0, 1, 12, 13):
        _QP_ENG[_ai] = "dve"
    else:
        _QP_ENG[_ai] = ("act", "act", "dve")[_c % 3]
        _c += 1


def build(L=4096, has_qkv_b=False, has_proj_b=False):
    LCH = 512
    NCH = L // LCH
    NSUB = LCH // P  # 4

    nc = bacc.Bacc("TRN2", target_bir_lowering=False, debug=False)
    x_d = nc.dram_tensor("x", [L, DIM], F32, kind="ExternalInput").ap()
    qkvw_d = nc.dram_tensor("qkv_w", [3 * DIM, DIM], F32, kind="ExternalInput").ap()
    qkvb_d = nc.dram_tensor("qkv_b", [3 * DIM], F32, kind="ExternalInput").ap()
    projw_d = nc.dram_tensor("proj_w", [DIM, DIM], F32, kind="ExternalInput").ap()
    projb_d = nc.dram_tensor("proj_b", [DIM], F32, kind="ExternalInput").ap()
    pm_d = nc.dram_tensor("proj_mat", [M, D], F32, kind="ExternalInput").ap()
    y_d = nc.dram_tensor("y", [L, DIM], F32, kind="ExternalOutput").ap()

    with tile.TileContext(nc) as tc:
        with ExitStack() as ctx:
            _body(ctx, tc, x_d, qkvw_d, qkvb_d, projw_d, projb_d, pm_d, y_d,
                  L, LCH, NCH, NSUB, has_qkv_b, has_proj_b)
    nc.compile()
    return nc


def _dr_gemm(nc, out, whl, xhl, wcols, lt=None, bias=None):
    """Accumulating fp8 DoubleRow hi/lo-compensated GEMM over 768 contraction.

    whl/xhl: [128, KT, 2, *] fp8 with hi at [:,kk,0,:], lo at [:,kk,1,:].
    lt=None : out[wc, l]; stationary = whl cols wcols, moving = xhl  (kT/qT)
    lt given: out[l, wc]; stationary = xhl l-tile cols, moving = whl (v/y)
    """
    c0, c1 = wcols
    n = c1 - c0
    if lt is None:
        for i in range(KT // 2):
            for kk in (2 * i, 2 * i + 1):
                nc.tensor.matmul(
                    out, whl[:, kk, :, c0:c1],
                    xhl[:, kk, 0:1, :].to_broadcast([P, 2, out.shape[-1]]),
                    start=(kk == 0), stop=False, perf_mode=DR,
                )
            nc.tensor.matmul(
                out, whl[:, 2 * i : 2 * i + 2, 0, c0:c1],
                xhl[:, 2 * i : 2 * i + 2, 1, :],
                start=False, stop=(bias is None and i == KT // 2 - 1),
                perf_mode=DR,
            )
    else:
        l0 = lt * P
        for i in range(KT // 2):
            for kk in (2 * i, 2 * i + 1):
                nc.tensor.matmul(
                    out, xhl[:, kk, :, l0 : l0 + P],
                    whl[:, kk, 0:1, c0:c1].to_broadcast([P, 2, n]),
                    start=(kk == 0), stop=False, perf_mode=DR,
                )
            nc.tensor.matmul(
                out, xhl[:, 2 * i : 2 * i + 2, 0, l0 : l0 + P],
                whl[:, 2 * i : 2 * i + 2, 1, c0:c1],
                start=False, stop=(bias is None and i == KT // 2 - 1),
                perf_mode=DR,
            )
    if bias is not None:
        ones_row, brow = bias
        nc.tensor.matmul(out, ones_row, brow, start=False, stop=True)


def _body(ctx, tc, x_d, qkvw_d, qkvb_d, projw_d, projb_d, pm_d, y_d,
          L, LCH, NCH, NSUB, has_qkv_b, has_proj_b):
    nc = tc.nc
    iqkv = 1.0 / (SX * SW)
    iy = 1.0 / (SA * SP)

    persist = ctx.enter_context(tc.tile_pool(name="persist", bufs=1))

    ident16 = persist.tile([P, P], F16, tag="ident16", name="ident16")[:]
    nc.gpsimd.memset(ident16, 0.0)
    nc.gpsimd.affine_select(
        out=ident16, in_=ident16, compare_op=AL.not_equal, fill=1.0,
        base=0, pattern=[[-1, P]], channel_multiplier=1,
    )
    ones1 = persist.tile([1, P], F16, tag="ones1", name="ones1")[:]
    nc.gpsimd.memset(ones1, 1.0)
    epscol = persist.tile([P, 1], F16, tag="epscol", name="epscol")[:]
    nc.gpsimd.memset(epscol, EPS)

    whl_qk = persist.tile([P, KT, 2, 2 * DIM], F8, tag="whl_qk", name="whl_qk")[:]
    wvhl = persist.tile([P, KT, 2, DIM], F8, tag="wvhl", name="wvhl")[:]
    wphl = persist.tile([P, KT, 2, DIM], F8, tag="wphl", name="wphl")[:]
    # folded blockdiag pm for the fp8 DoubleRow k-feature GEMM:
    # slice 0 = [RATIO*pmT | 0] (c rows 0:64), slice 1 = [0 | RATIO*pmT]
    pmbd = persist.tile([P, 2, 2 * M], F8, tag="pmbd", name="pmbd")[:]
    pmt2 = persist.tile([P, M], F16, tag="pmt2", name="pmt2")[:]
    qt_sb = persist.tile([P, NPAIR, L], F16, tag="qt_sb", name="qt_sb")[:]
    kvm = persist.tile([P, 4, 6, D + 1], F16, tag="kvm", name="kvm")[:]
    kvmcs = persist.tile([1, 4, 6, D + 1], F16, tag="kvmcs", name="kvmcs")[:]

    if has_qkv_b:
        qkb = persist.tile([P, 2 * KT], F32, tag="qkb", name="qkb")[:]
        nc.sync.dma_start(qkb, qkvb_d.rearrange("(t p) -> p t", p=P)[:, 0 : 2 * KT])
        qkbk = persist.tile([P, KT], F32, tag="qkbk", name="qkbk")[:]
        nc.scalar.activation(qkbk, qkb[:, KT : 2 * KT], AF.Copy, scale=SKT)
        vbf = persist.tile([1, DIM], F32, tag="vbf", name="vbf")[:]
        nc.sync.dma_start(vbf, qkvb_d[2 * DIM : 3 * DIM].unsqueeze(0))
        vb_row = persist.tile([1, DIM], F16, tag="vb_row", name="vb_row")[:]
        nc.scalar.activation(vb_row, vbf, AF.Copy, scale=SX * SW)
    if has_proj_b:
        pbf = persist.tile([1, DIM], F32, tag="pbf", name="pbf")[:]
        nc.sync.dma_start(pbf, projb_d.unsqueeze(0))
        pb_row = persist.tile([1, DIM], F16, tag="pb_row", name="pb_row")[:]
        nc.scalar.activation(pb_row, pbf, AF.Copy, scale=SA * SP)

    vsb = persist.tile([P, 2, NSUB, H, D + 1], F16, tag="vsb", name="vsb")[:]
    nc.gpsimd.memset(vsb[:, :, :, :, D : D + 1], 1.0)

    # kv accumulator in SBUF fp32 (indexed by ai = h*2+mt)
    kv_acc = persist.tile([P, 2 * H, D + 1], F32, tag="kv_acc", name="kv_acc")[:]

    # pass-2 q-feature tiles, double-buffered by chunk parity
    qp2 = [persist.tile([P, H, 2, LCH], F16, tag=f"qp2_{i}", name=f"qp2_{i}")[:]
           for i in range(2)]

    def emit_qp(ich, heads, psum_pool, tag):
        l0 = ich * LCH
        qp_sb = qp2[ich % 2]
        for h in heads:
            p, h2 = h // 2, h % 2
            for mt in range(2):
                qps = psum_pool.tile([P, LCH], F32, tag=tag, name="qps")[:]
                nc.tensor.matmul(
                    qps,
                    pmt2[h2 * D : (h2 + 1) * D, mt * P : (mt + 1) * P],
                    qt_sb[h2 * D : (h2 + 1) * D, p, l0 : l0 + LCH],
                    start=True, stop=True)
                eng = _QP_ENG[h * 2 + mt]
                if eng == "act":
                    # plain relu; eps restored via rank-1 kvmcs in num
                    nc.scalar.activation(qp_sb[:, h, mt, :], qps, AF.Relu)
                else:
                    nc.vector.tensor_scalar(qp_sb[:, h, mt, :], qps,
                                            EPS, EPS, AL.add, AL.max)

    if True:
        with tc.tile_pool(name="p1x", bufs=2) as xp, \
             tc.tile_pool(name="p1xhl", bufs=2) as xhlp, \
             tc.tile_pool(name="p1kt", bufs=3) as ktp, \
             tc.tile_pool(name="p1kp", bufs=8) as kpp, \
             tc.tile_pool(name="wprep", bufs=3) as wpool, \
             tc.tile_pool(name="p1work", bufs=8, space="PSUM") as work:

            xnats = {}
            xhls = {}

            def dma_x(ich):
                l0 = ich * LCH
                xnat = xp.tile([P, NSUB, DIM], F16, tag="xnat", name="xnat")[:]
                nc.gpsimd.dma_start(
                    xnat,
                    x_d[l0 : l0 + LCH, :].rearrange("(s p) k -> p s k", p=P))
                xnats[ich] = xnat

            xhl_cur = {}

            def transp_x(ich, kks=range(KT)):
                if ich in xhl_cur:
                    xnat, xhl = xhl_cur[ich]
                else:
                    xnat = xnats.pop(ich)
                    xhl = xhlp.tile([P, KT, 2, LCH], F8, tag="xhl",
                                    name="xhl")[:]
                    xhl_cur[ich] = (xnat, xhl)
                for kk in kks:
                    tp = work.tile([P, 512], F16, tag="work", name="ttp")[:]
                    for s in range(NSUB):
                        nc.tensor.transpose(
                            tp[:, s * P : (s + 1) * P],
                            xnat[:, s, kk * P : (kk + 1) * P], ident16)
                    nc.scalar.activation(xhl[:, kk, 0, :], tp, AF.Copy,
                                         scale=SX)
                    nc.vector.scalar_tensor_tensor(
                        xhl[:, kk, 1, :], tp, SX, xhl[:, kk, 0, :],
                        AL.mult, AL.subtract)
                xhls[ich] = xhl

            def emit_v(ich, subs):
                vs = ich % 2
                xhl = xhls[ich]
                for s in subs:
                    for ci in range(2):
                        c0 = ci * 384
                        vps = work.tile([P, 512], F32, tag="work", name="vps")[:]
                        bias = None
                        if has_qkv_b:
                            bias = (ones1, vb_row[:, c0 : c0 + 384])
                        _dr_gemm(nc, vps[:, 0:384], wvhl, xhl, (c0, c0 + 384),
                                 lt=s, bias=bias)
                        nc.scalar.activation(
                            vsb[:, vs, s, 6 * ci : 6 * ci + 6, 0:D],
                            vps[:, 0:384].rearrange("p (h d) -> p h d", d=D),
                            AF.Copy, scale=iqkv)

            def emit_kT(ich, p):
                ktps = work.tile([P, 512], F32, tag="work", name="ktps")[:]
                _dr_gemm(nc, ktps, whl_qk, xhls[ich],
                         (DIM + p * P, DIM + (p + 1) * P))
                # fold [128,512] -> [64, 2, 512] fp8 (scaled) for DoubleRow
                kt = ktp.tile([P, 2, LCH], F8, tag="kt", name="kt")[:]
                for h2 in range(2):
                    if has_qkv_b:
                        nc.scalar.activation(
                            kt[0:D, h2, :], ktps[h2 * D : (h2 + 1) * D, :],
                            AF.Identity,
                            bias=qkbk[h2 * D : (h2 + 1) * D, p : p + 1],
                            scale=SKT * iqkv)
                    else:
                        nc.scalar.activation(
                            kt[0:D, h2, :], ktps[h2 * D : (h2 + 1) * D, :],
                            AF.Copy, scale=SKT * iqkv)
                return kt

            def emit_kp(kt):
                kps = []
                for lt in range(NSUB):
                    kpps = work.tile([P, 512], F32, tag="work", name="kpps")[:]
                    nc.tensor.matmul(kpps,
                                     kt[0:D, :, lt * P : (lt + 1) * P],
                                     pmbd[0:D], start=True, stop=True,
                                     perf_mode=DR)
                    kp = kpp.tile([P, 2 * M], F16, tag="kp", name="kp")[:]
                    # k-feature path is scaled by SKP; it cancels in num/den
                    nc.vector.tensor_scalar(kp, kpps, SKP * EPS, SKP * EPS,
                                            AL.add, AL.max)
                    kps.append(kp)
                return kps

            def emit_qT(ich, p):
                l0 = ich * LCH
                qtps = work.tile([P, 512], F32, tag="work", name="qtps")[:]
                _dr_gemm(nc, qtps, whl_qk, xhls[ich], (p * P, (p + 1) * P))
                if has_qkv_b:
                    nc.scalar.activation(qt_sb[:, p, l0 : l0 + LCH], qtps,
                                         AF.Identity,
                                         bias=qkb[:, p : p + 1], scale=iqkv)
                else:
                    nc.scalar.activation(qt_sb[:, p, l0 : l0 + LCH], qtps,
                                         AF.Copy, scale=iqkv)

            def emit_kv(ich, p, kps):
                vs = ich % 2
                kvp = work.tile([P, 4, D + 1], F32, tag="work", name="kvp")[:]
                for lt in range(NSUB):
                    kp = kps[lt]
                    for h2 in range(2):
                        h = 2 * p + h2
                        for mt in range(2):
                            j = h2 * 2 + mt
                            nc.tensor.matmul(
                                kvp[:, j, :],
                                kp[:, j * P : (j + 1) * P],
                                vsb[:, vs, lt, h, :],
                                start=(lt == 0 and j == 0),
                                stop=(lt == NSUB - 1 and j == 3),
                            )
                nc.vector.tensor_tensor(
                    kv_acc[:, 4 * p : 4 * p + 4, :], kvp,
                    kv_acc[:, 4 * p : 4 * p + 4, :], AL.add)

            # ---- prep: weight DMA blocks interleaved with chunk-0/1 work ----
            def prep_w_blocks(src, nrows, dst, dst_off, scale):
                blocks = []
                nt = nrows // P
                c0 = 0
                while c0 < nt:
                    bs = min(3, nt - c0)
                    st = {}

                    def bdma(c0=c0, bs=bs, st=st):
                        wnat = wpool.tile([P, 3, DIM], F16, tag="wnat",
                                          name="wnat")[:]
                        nc.gpsimd.dma_start(
                            wnat[:, 0:bs, :],
                            src[c0 * P : (c0 + bs) * P, :].rearrange(
                                "(s p) k -> p s k", p=P))
                        st["wnat"] = wnat

                    def bcomp(c0=c0, bs=bs, st=st):
                        wnat = st["wnat"]
                        for kk in range(KT):
                            tp = work.tile([P, 512], F16, tag="work",
                                           name="ptp")[:]
                            for j in range(bs):
                                nc.tensor.transpose(
                                    tp[:, j * P : (j + 1) * P],
                                    wnat[:, j, kk * P : (kk + 1) * P], ident16)
                            hi = dst[:, kk, 0,
                                     dst_off + c0 * P : dst_off + (c0 + bs) * P]
                            nc.scalar.activation(hi, tp[:, 0 : bs * P], AF.Copy,
                                                 scale=scale)
                            nc.vector.scalar_tensor_tensor(
                                dst[:, kk, 1,
                                    dst_off + c0 * P : dst_off + (c0 + bs) * P],
                                tp[:, 0 : bs * P], scale, hi,
                                AL.mult, AL.subtract)

                    blocks.append((bdma, bcomp))
                    c0 += bs
                return blocks

            pm_st = {}

            def prep_pm_dma():
                pmn = wpool.tile([P, 2, D], F16, tag="pmn", name="pmn")[:]
                nc.gpsimd.dma_start(pmn, pm_d.rearrange("(s p) d -> p s d", p=P))
                pm_st["pmn"] = pmn

            def prep_pm():
                pmn = pm_st["pmn"]
                tp = work.tile([P, 512], F16, tag="work", name="ptp")[:]
                for s in range(2):
                    nc.tensor.transpose(tp[0:D, s * P : (s + 1) * P],
                                        pmn[:, s, :], ident16)
                nc.gpsimd.memset(pmbd, 0.0)
                nc.scalar.activation(pmbd[0:D, 0, 0:M], tp[0:D, 0:M], AF.Copy,
                                     scale=SPM * RATIO)
                nc.scalar.activation(pmbd[0:D, 1, M : 2 * M], tp[0:D, 0:M],
                                     AF.Copy, scale=SPM * RATIO)
                nc.scalar.activation(pmt2[0:D, :], tp[0:D, 0:M], AF.Copy,
                                     scale=RATIO)
                nc.scalar.activation(pmt2[D:P, :], tp[0:D, 0:M], AF.Copy,
                                     scale=RATIO)

            dma_x(0)
            wv = prep_w_blocks(qkvw_d[2 * DIM : 3 * DIM, :], DIM, wvhl, 0, SW)
            wqk_k = prep_w_blocks(qkvw_d[DIM : 2 * DIM, :], DIM, whl_qk,
                                  DIM, SW)
            wqk_q = prep_w_blocks(qkvw_d[0:DIM, :], DIM, whl_qk, 0, SW)
            wp = prep_w_blocks(projw_d, DIM, wphl, 0, SP)

            nc.gpsimd.memset(kv_acc, 0.0)
            blocks = wv + wqk_k + wqk_q + wp
            bst = {"dma": 0, "comp": 0}

            def bdma_next():
                if bst["dma"] < len(blocks):
                    blocks[bst["dma"]][0]()
                    bst["dma"] += 1

            def bcomp_next():
                if bst["comp"] < len(blocks):
                    blocks[bst["comp"]][1]()
                    bst["comp"] += 1
                    bdma_next()

            nop = lambda: None
            # phase A: wv + k-part of wqk; q-part and proj stream into chunk 0
            nA = len(wv) + len(wqk_k)
            fillers = {
                0: [lambda: transp_x(0, range(0, 3)),
                    (lambda: dma_x(1)) if 1 < NCH else nop],
                1: [lambda: transp_x(0, range(3, KT)), prep_pm],
                2: [lambda: emit_v(0, (0,)), lambda: emit_v(0, (1,)),
                    (lambda: transp_x(1, range(0, 3))) if 1 < NCH else nop],
                3: [lambda: emit_v(0, (2,)), lambda: emit_v(0, (3,)),
                    (lambda: transp_x(1, range(3, KT))) if 1 < NCH else nop,
                    (lambda: dma_x(2)) if 2 < NCH else nop],
            }
            bdma_next()
            prep_pm_dma()
            bdma_next()
            # PE p-state warmup: burn the cold DMA-wait ramping the clock
            wu = work.tile([P, 512], F16, tag="work", name="wu")[:]
            for _ in range(7):
                for s in range(4):
                    nc.tensor.transpose(wu[:, s * P : (s + 1) * P], ident16,
                                        ident16)
            for i in range(nA):
                for f in fillers.get(i, []):
                    f()
                bcomp_next()

            # ---- pass 1 main loop ----
            for ich in range(NCH):
                first = ich == 0
                pend_kt = None
                pend = None
                for p in range(NPAIR):
                    kt = emit_kT(ich, p)
                    if first:
                        # stream remaining weight blocks (q-part + proj)
                        bcomp_next()
                        if p >= 3:
                            emit_qT(0, p - 3)
                    else:
                        emit_qT(ich, p)
                    if pend is not None:
                        emit_kv(ich, pend[0], pend[1])
                        if ich == NCH - 1:
                            pp = pend[0]
                            nc.scalar.activation(
                                kvm.rearrange("p b j c -> p (b j) c")[
                                    :, 4 * pp : 4 * pp + 4, :],
                                kv_acc[:, 4 * pp : 4 * pp + 4, :],
                                AF.Copy, scale=1.0 / SKP)
                        pend = None
                    if pend_kt is not None:
                        pend = (pend_kt[0], emit_kp(pend_kt[1]))
                    pend_kt = (p, kt)
                    if ich == NCH - 1 and not first:
                        # chunk-0 q features computed here so pass 2 starts hot
                        emit_qp(0, (2 * p, 2 * p + 1), work, "work")
                    if p == 0 and 1 <= ich and ich + 1 < NCH:
                        transp_x(ich + 1, range(0, 3))
                    if p == 2 and 1 <= ich and ich + 1 < NCH:
                        transp_x(ich + 1, range(3, KT))
                    if p == 3 and 1 <= ich and ich + 2 < NCH:
                        dma_x(ich + 2)
                    if p >= 3 and ich + 1 < NCH:
                        emit_v(ich + 1, (p - 3,))
                if pend is not None:
                    emit_kv(ich, pend[0], pend[1])
                    if ich == NCH - 1:
                        pp = pend[0]
                        nc.scalar.activation(
                            kvm.rearrange("p b j c -> p (b j) c")[
                                :, 4 * pp : 4 * pp + 4, :],
                            kv_acc[:, 4 * pp : 4 * pp + 4, :],
                            AF.Copy, scale=1.0 / SKP)
                pend = (pend_kt[0], emit_kp(pend_kt[1]))
                if ich + 1 < NCH:
                    emit_v(ich + 1, (3,))
                emit_kv(ich, pend[0], pend[1])
                if ich == NCH - 1:
                    nc.scalar.activation(
                        kvm.rearrange("p b j c -> p (b j) c")[:, 20:24, :],
                        kv_acc[:, 20:24, :], AF.Copy, scale=1.0 / SKP)
                if first:
                    for p3 in range(3, NPAIR):
                        emit_qT(0, p3)
                    if NCH == 1:
                        emit_qp(0, range(H), work, "work")
                xhls.pop(ich)

    with tc.tile_pool(name="csps", bufs=1, space="PSUM") as cspool:
        css = []
        for b in range(4):
            cs = cspool.tile([1, 6, D + 1], F32, tag=f"cs{b}", name="cs")[:]
            for j in range(6):
                nc.tensor.matmul(cs[:, j, :], epscol, kvm[:, b, j, :],
                                 start=(j == 0), stop=(j == 5))
            css.append(cs)
        for b in range(4):
            nc.scalar.copy(kvmcs[:, b], css[b])

    # ---- pass 2 ----
    with tc.tile_pool(name="p2attn", bufs=2) as atp_sb, \
         tc.tile_pool(name="p2rd", bufs=2) as rdp, \
         tc.tile_pool(name="p2ahl", bufs=2) as ahlp, \
         tc.tile_pool(name="p2y", bufs=2) as yp, \
         tc.tile_pool(name="ps2qp", bufs=3, space="PSUM") as qppsum, \
         tc.tile_pool(name="ps2nm", bufs=2, space="PSUM") as numpsum, \
         tc.tile_pool(name="ps2at", bufs=1, space="PSUM") as atpsum, \
         tc.tile_pool(name="ps2y", bufs=1, space="PSUM") as ypsum:

        def emit_y(ich, ahl, lt):
            l0 = ich * LCH
            yps = ypsum.tile([P, DIM], F32, tag="yps", name="yps")[:]
            for c0, c1 in ((0, 512), (512, DIM)):
                b = (ones1, pb_row[:, c0:c1]) if has_proj_b else None
                _dr_gemm(nc, yps[:, c0:c1], wphl, ahl, (c0, c1), lt=lt, bias=b)
            ysb = yp.tile([P, DIM], F32, tag="ysb", name="ysb")[:]
            nc.scalar.activation(ysb, yps, AF.Copy, scale=iy)
            nc.sync.dma_start(y_d[l0 + lt * P : l0 + (lt + 1) * P, :], ysb)

        pend_y = []
        for ich in range(NCH):
            qp_sb = qp2[ich % 2]
            ahl = ahlp.tile([P, KT, 2, LCH], F8, tag="ahl", name="ahl")[:]
            for lt in range(NSUB):
                nmps = [numpsum.tile([P, 6, D + 1], F32, tag="nm", name="nmps")[:]
                        for _ in range(2)]
                for h in range(H):
                    g = h // 6
                    for mt in range(2):
                        ai = h * 2 + mt
                        if _QP_ENG[ai] == "act" and h % 6 != 0:
                            nc.tensor.matmul(
                                nmps[g][:, h % 6, :], ones1,
                                kvmcs[0:1, ai // 6, ai % 6, :],
                                start=False, stop=False)
                    for mt in range(2):
                        ai = h * 2 + mt
                        nc.tensor.matmul(
                            nmps[g][:, h % 6, :],
                            qp_sb[:, h, mt, lt * P : (lt + 1) * P],
                            kvm[:, ai // 6, ai % 6, :],
                            start=(mt == 0 and h % 6 == 0),
                            stop=(mt == 1 and h % 6 == 5))
                rd = rdp.tile([P, H], F32, tag="rd", name="rd")[:]
                attn = atp_sb.tile([P, H, D], F16, tag="attn", name="attn")[:]
                for g in range(2):
                    nc.vector.reciprocal(rd[:, g * 6 : (g + 1) * 6],
                                         nmps[g][:, :, D])
                    nc.vector.tensor_tensor(
                        attn[:, g * 6 : (g + 1) * 6, :],
                        nmps[g][:, :, 0:D],
                        rd[:, g * 6 : (g + 1) * 6, None].to_broadcast([P, 6, D]),
                        AL.mult)
                if ich + 1 < NCH:
                    emit_qp(ich + 1, range(3 * lt, 3 * lt + 3), qppsum, "qps")
                if len(pend_y) >= (2 if ich + 1 < NCH else 1):
                    emit_y(*pend_y.pop(0))
                atps = atpsum.tile([P, DIM], F16, tag="at", name="atps")[:]
                for kk in range(KT):
                    nc.tensor.transpose(atps[:, kk * P : (kk + 1) * P],
                                        attn[:, 2 * kk : 2 * kk + 2, :], ident16)
                a3 = atps.rearrange("p (k l) -> p k l", k=KT)
                nc.scalar.activation(ahl[:, :, 0, lt * P : (lt + 1) * P], a3,
                                     AF.Copy, scale=SA)
                nc.vector.scalar_tensor_tensor(
                    ahl[:, :, 1, lt * P : (lt + 1) * P], a3, SA,
                    ahl[:, :, 0, lt * P : (lt + 1) * P], AL.mult, AL.subtract)
                pend_y.append((ich, ahl, lt))
        for args in pend_y:
            emit_y(*args)


_CACHE = {}


def _get_nc(L=4096, hqb=False, hpb=False):
    key = ("nc", L, hqb, hpb)
    if key not in _CACHE:
        _CACHE[key] = build(L, hqb, hpb)
    return _CACHE[key]


last_exec_time_ns = None
last_profile = None


def kernel(x, qkv_w, qkv_b, proj_w, proj_b, proj_mat):
    global last_exec_time_ns, last_profile
    from concourse.bass_utils import run_bass_kernel_spmd

    x = np.asarray(x, np.float32)
    B, L, _ = x.shape
    hqb = bool(np.any(np.asarray(qkv_b)))
    hpb = bool(np.any(np.asarray(proj_b)))
    nc = _get_nc(L, hqb, hpb)
    base = {
        "qkv_w": np.ascontiguousarray(np.asarray(qkv_w, np.float32)),
        "qkv_b": np.ascontiguousarray(np.asarray(qkv_b, np.float32)),
        "proj_w": np.ascontiguousarray(np.asarray(proj_w, np.float32)),
        "proj_b": np.ascontiguousarray(np.asarray(proj_b, np.float32)),
        "proj_mat": np.ascontiguousarray(np.asarray(proj_mat, np.float32)),
    }
    in_maps = [dict(base, x=np.ascontiguousarray(x[b])) for b in range(B)]
    trace = bool(int(os.environ.get("KERNEL_TRACE", "0")))
    res = run_bass_kernel_spmd(nc, in_maps, core_ids=list(range(B)), trace=trace)
    last_exec_time_ns = res.exec_time_ns
    last_profile = res.profile_json
    return np.stack([res.results[b]["y"] for b in range(B)], axis=0)


def _ref_np(x, qkv_w, qkv_b, proj_w, proj_b, proj_mat):
    Ls = x.shape[0]
    qkv = x @ qkv_w.T + qkv_b
    qkv = qkv.reshape(Ls, 3, H, D)
    q, k, v = qkv[:, 0], qkv[:, 1], qkv[:, 2]
    qp = np.maximum(RATIO * np.einsum("lhd,md->lhm", q, proj_mat), 0) + EPS
    kp = np.maximum(RATIO * np.einsum("lhd,md->lhm", k, proj_mat), 0) + EPS
    kv = np.einsum("lhm,lhd->hmd", kp, v)
    ks = kp.sum(axis=0)
    num = np.einsum("lhm,hmd->lhd", qp, kv)
    den = np.einsum("lhm,hm->lh", qp, ks)
    out = (num / den[..., None]).reshape(Ls, DIM)
    return out @ proj_w.T + proj_b


if __name__ == "__main__":
    from concourse.bass_interp import CoreSim

    Ls = int(os.environ.get("SIM_L", "512"))
    use_bias = bool(int(os.environ.get("SIM_BIAS", "1")))
    rng = np.random.default_rng(0)
    x = rng.standard_normal((Ls, DIM), dtype=np.float32)
    qkv_w = (rng.standard_normal((3 * DIM, DIM), dtype=np.float32) * DIM**-0.5)
    qkv_b = (rng.standard_normal(3 * DIM, dtype=np.float32) * 0.1
             if use_bias else np.zeros(3 * DIM, np.float32))
    proj_w = (rng.standard_normal((DIM, DIM), dtype=np.float32) * DIM**-0.5)
    proj_b = (rng.standard_normal(DIM, dtype=np.float32) * 0.1
              if use_bias else np.zeros(DIM, np.float32))
    proj_mat = rng.standard_normal((M, D), dtype=np.float32)

    print(f"building L={Ls} bias={use_bias} ...")
    nc = build(Ls, use_bias, use_bias)
    print("simulating ...")
    sim = CoreSim(nc)
    for name, arr in [("x", x), ("qkv_w", qkv_w), ("qkv_b", qkv_b),
                      ("proj_w", proj_w), ("proj_b", proj_b),
                      ("proj_mat", proj_mat)]:
        sim.tensor(name)[:] = arr
    sim.simulate(check_with_hw=False)
    got = np.array(sim.tensor("y"))
    want = _ref_np(x, qkv_w, qkv_b, proj_w, proj_b, proj_mat)
    err = np.abs(got - want)
    rel = np.linalg.norm(got - want) / np.linalg.norm(want)
    print("max abs err:", err.max(), " rel fro err:", rel)
    assert rel < 2e-2, "sim mismatch"
    print("SIM OK")


# revision 12
# speedup vs baseline: 1.0229x; 1.0006x over previous
"""FAVOR+ (Performer) non-causal linear attention on 8 Trainium2 NeuronCores.

Sharding: data-parallel over batch B=8 -> one batch element per core.

Per-core pipeline (L=4096, DIM=768, H=12, D=64, M=256):
  prep : cast-DMA weights to fp16, PE-transpose to feature-major, split into
         fp8e4m3 hi/lo pairs (scaled) for DoubleRow matmuls; DMA blocks
         interleaved with chunk-0/1 transposes and v so PE never starves
  pass1: per 512-row chunk: cast-DMA x to fp16; PE-transpose -> xT hi/lo fp8;
         kT/qT/v via fp8 DoubleRow hi/lo-compensated GEMMs (4.5 cyc per
         128x512 output tile instead of 6); k features fp16 with blockdiag pm
         (2 heads per matmul); kv accumulated m-major [m,65] into 4 persistent
         PSUM banks across all chunks (ones-augmented v gives k_sum for
         free); qT staged in SBUF fp16 (no DRAM round trip).  Emission is
         software-pipelined: transposes/v run 2 chunks ahead; kv trails one
         pair behind its kp conversion.
  mid  : kv PSUM -> fp16 SBUF (already m-major); eps*colsum(kv) rows for the
         ACT-assigned q-feature heads
  pass2: q features m-major fp16 (computed one chunk ahead, spread across the
         l-tile loop; relu+eps on DVE/Pool, plain relu on ACT with the eps
         restored by a rank-1 matmul into num); num L-major [l,65] (den =
         col 64); reciprocal + broadcast multiply on DVE; attn PE-transposed
         to feature-major, split fp8 hi/lo; y via DoubleRow GEMM -> DMA out
"""

import math
import os
import sys
from contextlib import ExitStack

import numpy as np

for _p in ("/opt/trn_rl_repo",):
    if _p not in sys.path and os.path.isdir(_p):
        sys.path.insert(0, _p)

import concourse.bass as bass  # noqa: E402
import concourse.mybir as mybir  # noqa: E402
import concourse.tile as tile  # noqa: E402
from concourse import bacc  # noqa: E402

P = 128
DIM = 768
H = 12
D = 64
M = 256
KT = DIM // P   # 6 contraction c-tiles
NPAIR = H // 2  # 6 head pairs
EPS = 1e-3
RATIO = 1.0 / math.sqrt(float(M))

SX = 16.0    # x ~ N(0,1)
SW = 32.0    # qkv_w ~ N(0, 1/768)
SA = 64.0    # attn ~ 0.1
SP = 32.0    # proj_w ~ N(0, 1/768)
SKT = 16.0   # kT ~ N(0,1) -> fp8 for the k-feature GEMM
SPM = 32.0   # RATIO*pm ~ N(0, 1/256) -> fp8
SKP = SKT * SPM  # k-feature path runs in this scaled domain until kvm

F32 = mybir.dt.float32
F16 = mybir.dt.float16
F8 = mybir.dt.float8e4
AL = mybir.AluOpType
AF = mybir.ActivationFunctionType
DR = mybir.MatmulPerfMode.DoubleRow

# pass-2 q-feature conversion engine per (head, mtile) slot ai=h*2+mt.
# Slots of the group-starting heads (ai 0,1,12,13) stay off ACT so each num
# PSUM group's first instruction is its start matmul.
_QP_ENG = {}
_c = 0
for _ai in range(2 * H):
    if _ai in (0, 1, 12, 13):
        _QP_ENG[_ai] = "dve"
    else:
        _QP_ENG[_ai] = ("act", "act", "dve")[_c % 3]
        _c += 1


def build(L=4096, has_qkv_b=False, has_proj_b=False):
    LCH = 512
    NCH = L // LCH
    NSUB = LCH // P  # 4

    nc = bacc.Bacc("TRN2", target_bir_lowering=False, debug=False)
    x_d = nc.dram_tensor("x", [L, DIM], F32, kind="ExternalInput").ap()
    qkvw_d = nc.dram_tensor("qkv_w", [3 * DIM, DIM], F32, kind="ExternalInput").ap()
    qkvb_d = nc.dram_tensor("qkv_b", [3 * DIM], F32, kind="ExternalInput").ap()
    projw_d = nc.dram_tensor("proj_w", [DIM, DIM], F32, kind="ExternalInput").ap()
    projb_d = nc.dram_tensor("proj_b", [DIM], F32, kind="ExternalInput").ap()
    pm_d = nc.dram_tensor("proj_mat", [M, D], F32, kind="ExternalInput").ap()
    y_d = nc.dram_tensor("y", [L, DIM], F32, kind="ExternalOutput").ap()

    with tile.TileContext(nc) as tc:
        with ExitStack() as ctx:
            _body(ctx, tc, x_d, qkvw_d, qkvb_d, projw_d, projb_d, pm_d, y_d,
                  L, LCH, NCH, NSUB, has_qkv_b, has_proj_b)
    nc.compile()
    return nc


def _dr_gemm(nc, out, whl, xhl, wcols, lt=None, bias=None):
    """Accumulating fp8 DoubleRow hi/lo-compensated GEMM over 768 contraction.

    whl/xhl: [128, KT, 2, *] fp8 with hi at [:,kk,0,:], lo at [:,kk,1,:].
    lt=None : out[wc, l]; stationary = whl cols wcols, moving = xhl  (kT/qT)
    lt given: out[l, wc]; stationary = xhl l-tile cols, moving = whl (v/y)
    """
    c0, c1 = wcols
    n = c1 - c0
    if lt is None:
        for i in range(KT // 2):
            for kk in (2 * i, 2 * i + 1):
                nc.tensor.matmul(
                    out, whl[:, kk, :, c0:c1],
                    xhl[:, kk, 0:1, :].to_broadcast([P, 2, out.shape[-1]]),
                    start=(kk == 0), stop=False, perf_mode=DR,
                )
            nc.tensor.matmul(
                out, whl[:, 2 * i : 2 * i + 2, 0, c0:c1],
                xhl[:, 2 * i : 2 * i + 2, 1, :],
                start=False, stop=(bias is None and i == KT // 2 - 1),
                perf_mode=DR,
            )
    else:
        l0 = lt * P
        for i in range(KT // 2):
            for kk in (2 * i, 2 * i + 1):
                nc.tensor.matmul(
                    out, xhl[:, kk, :, l0 : l0 + P],
                    whl[:, kk, 0:1, c0:c1].to_broadcast([P, 2, n]),
                    start=(kk == 0), stop=False, perf_mode=DR,
                )
            nc.tensor.matmul(
                out, xhl[:, 2 * i : 2 * i + 2, 0, l0 : l0 + P],
                whl[:, 2 * i : 2 * i + 2, 1, c0:c1],
                start=False, stop=(bias is None and i == KT // 2 - 1),
                perf_mode=DR,
            )
    if bias is not None:
        ones_row, brow = bias
        nc.tensor.matmul(out, ones_row, brow, start=False, stop=True)


def _body(ctx, tc, x_d, qkvw_d, qkvb_d, projw_d, projb_d, pm_d, y_d,
          L, LCH, NCH, NSUB, has_qkv_b, has_proj_b):
    nc = tc.nc
    iqkv = 1.0 / (SX * SW)
    iy = 1.0 / (SA * SP)

    persist = ctx.enter_context(tc.tile_pool(name="persist", bufs=1))

    ident16 = persist.tile([P, P], F16, tag="ident16", name="ident16")[:]
    nc.gpsimd.memset(ident16, 0.0)
    nc.gpsimd.affine_select(
        out=ident16, in_=ident16, compare_op=AL.not_equal, fill=1.0,
        base=0, pattern=[[-1, P]], channel_multiplier=1,
    )
    ones1 = persist.tile([1, P], F16, tag="ones1", name="ones1")[:]
    nc.gpsimd.memset(ones1, 1.0)
    epscol = persist.tile([P, 1], F16, tag="epscol", name="epscol")[:]
    nc.gpsimd.memset(epscol, EPS)
    epsb = persist.tile([P, 1], F32, tag="epsb", name="epsb")[:]
    nc.gpsimd.memset(epsb, SKP * EPS)

    whl_qk = persist.tile([P, KT, 2, 2 * DIM], F8, tag="whl_qk", name="whl_qk")[:]
    wvhl = persist.tile([P, KT, 2, DIM], F8, tag="wvhl", name="wvhl")[:]
    wphl = persist.tile([P, KT, 2, DIM], F8, tag="wphl", name="wphl")[:]
    # folded blockdiag pm for the fp8 DoubleRow k-feature GEMM:
    # slice 0 = [RATIO*pmT | 0] (c rows 0:64), slice 1 = [0 | RATIO*pmT]
    pmbd = persist.tile([P, 2, 2 * M], F8, tag="pmbd", name="pmbd")[:]
    pmt2 = persist.tile([P, M], F16, tag="pmt2", name="pmt2")[:]
    qt_sb = persist.tile([P, NPAIR, L], F16, tag="qt_sb", name="qt_sb")[:]
    kvm = persist.tile([P, 4, 6, D + 1], F16, tag="kvm", name="kvm")[:]
    kvmcs = persist.tile([1, 4, 6, D + 1], F16, tag="kvmcs", name="kvmcs")[:]

    if has_qkv_b:
        qkb = persist.tile([P, 2 * KT], F32, tag="qkb", name="qkb")[:]
        nc.sync.dma_start(qkb, qkvb_d.rearrange("(t p) -> p t", p=P)[:, 0 : 2 * KT])
        qkbk = persist.tile([P, KT], F32, tag="qkbk", name="qkbk")[:]
        nc.scalar.activation(qkbk, qkb[:, KT : 2 * KT], AF.Copy, scale=SKT)
        vbf = persist.tile([1, DIM], F32, tag="vbf", name="vbf")[:]
        nc.sync.dma_start(vbf, qkvb_d[2 * DIM : 3 * DIM].unsqueeze(0))
        vb_row = persist.tile([1, DIM], F16, tag="vb_row", name="vb_row")[:]
        nc.scalar.activation(vb_row, vbf, AF.Copy, scale=SX * SW)
    if has_proj_b:
        pbf = persist.tile([1, DIM], F32, tag="pbf", name="pbf")[:]
        nc.sync.dma_start(pbf, projb_d.unsqueeze(0))
        pb_row = persist.tile([1, DIM], F16, tag="pb_row", name="pb_row")[:]
        nc.scalar.activation(pb_row, pbf, AF.Copy, scale=SA * SP)

    vsb = persist.tile([P, 2, NSUB, H, D + 1], F16, tag="vsb", name="vsb")[:]
    nc.gpsimd.memset(vsb[:, :, :, :, D : D + 1], 1.0)

    # kv accumulator in SBUF fp32 (indexed by ai = h*2+mt)
    kv_acc = persist.tile([P, 2 * H, D + 1], F32, tag="kv_acc", name="kv_acc")[:]

    # pass-2 q-feature tiles, double-buffered by chunk parity
    qp2 = [persist.tile([P, H, 2, LCH], F16, tag=f"qp2_{i}", name=f"qp2_{i}")[:]
           for i in range(2)]

    def emit_qp(ich, heads, psum_pool, tag):
        l0 = ich * LCH
        qp_sb = qp2[ich % 2]
        for h in heads:
            p, h2 = h // 2, h % 2
            for mt in range(2):
                qps = psum_pool.tile([P, LCH], F32, tag=tag, name="qps")[:]
                nc.tensor.matmul(
                    qps,
                    pmt2[h2 * D : (h2 + 1) * D, mt * P : (mt + 1) * P],
                    qt_sb[h2 * D : (h2 + 1) * D, p, l0 : l0 + LCH],
                    start=True, stop=True)
                eng = _QP_ENG[h * 2 + mt]
                if eng == "act":
                    # plain relu; eps restored via rank-1 kvmcs in num
                    nc.scalar.activation(qp_sb[:, h, mt, :], qps, AF.Relu)
                else:
                    nc.vector.tensor_scalar(qp_sb[:, h, mt, :], qps,
                                            EPS, EPS, AL.add, AL.max)

    if True:
        with tc.tile_pool(name="p1x", bufs=2) as xp, \
             tc.tile_pool(name="p1xhl", bufs=2) as xhlp, \
             tc.tile_pool(name="p1kt", bufs=3) as ktp, \
             tc.tile_pool(name="p1kp", bufs=8) as kpp, \
             tc.tile_pool(name="wprep", bufs=3) as wpool, \
             tc.tile_pool(name="p1work", bufs=8, space="PSUM") as work:

            xnats = {}
            xhls = {}

            def dma_x(ich):
                l0 = ich * LCH
                xnat = xp.tile([P, NSUB, DIM], F16, tag="xnat", name="xnat")[:]
                nc.gpsimd.dma_start(
                    xnat,
                    x_d[l0 : l0 + LCH, :].rearrange("(s p) k -> p s k", p=P))
                xnats[ich] = xnat

            xhl_cur = {}

            def transp_x(ich, kks=range(KT)):
                if ich in xhl_cur:
                    xnat, xhl = xhl_cur[ich]
                else:
                    xnat = xnats.pop(ich)
                    xhl = xhlp.tile([P, KT, 2, LCH], F8, tag="xhl",
                                    name="xhl")[:]
                    xhl_cur[ich] = (xnat, xhl)
                for kk in kks:
                    tp = work.tile([P, 512], F16, tag="work", name="ttp")[:]
                    for s in range(NSUB):
                        nc.tensor.transpose(
                            tp[:, s * P : (s + 1) * P],
                            xnat[:, s, kk * P : (kk + 1) * P], ident16)
                    nc.scalar.activation(xhl[:, kk, 0, :], tp, AF.Copy,
                                         scale=SX)
                    nc.vector.scalar_tensor_tensor(
                        xhl[:, kk, 1, :], tp, SX, xhl[:, kk, 0, :],
                        AL.mult, AL.subtract)
                xhls[ich] = xhl

            def emit_v(ich, subs):
                vs = ich % 2
                xhl = xhls[ich]
                for s in subs:
                    for ci in range(2):
                        c0 = ci * 384
                        vps = work.tile([P, 512], F32, tag="work", name="vps")[:]
                        bias = None
                        if has_qkv_b:
                            bias = (ones1, vb_row[:, c0 : c0 + 384])
                        _dr_gemm(nc, vps[:, 0:384], wvhl, xhl, (c0, c0 + 384),
                                 lt=s, bias=bias)
                        nc.scalar.activation(
                            vsb[:, vs, s, 6 * ci : 6 * ci + 6, 0:D],
                            vps[:, 0:384].rearrange("p (h d) -> p h d", d=D),
                            AF.Copy, scale=iqkv)

            def emit_kT(ich, p):
                ktps = work.tile([P, 512], F32, tag="work", name="ktps")[:]
                _dr_gemm(nc, ktps, whl_qk, xhls[ich],
                         (DIM + p * P, DIM + (p + 1) * P))
                # fold [128,512] -> [64, 2, 512] fp8 (scaled) for DoubleRow
                kt = ktp.tile([P, 2, LCH], F8, tag="kt", name="kt")[:]
                for h2 in range(2):
                    if has_qkv_b:
                        nc.scalar.activation(
                            kt[0:D, h2, :], ktps[h2 * D : (h2 + 1) * D, :],
                            AF.Identity,
                            bias=qkbk[h2 * D : (h2 + 1) * D, p : p + 1],
                            scale=SKT * iqkv)
                    else:
                        nc.scalar.activation(
                            kt[0:D, h2, :], ktps[h2 * D : (h2 + 1) * D, :],
                            AF.Copy, scale=SKT * iqkv)
                return kt

            def emit_kp(p, kt):
                kps = []
                for lt in range(NSUB):
                    kpps = work.tile([P, 512], F32, tag="work", name="kpps")[:]
                    nc.tensor.matmul(kpps,
                                     kt[0:D, :, lt * P : (lt + 1) * P],
                                     pmbd[0:D], start=True, stop=True,
                                     perf_mode=DR)
                    kp = kpp.tile([P, 2 * M], F16, tag="kp", name="kp")[:]
                    # k-feature path is scaled by SKP; it cancels in num/den.
                    # ACT-assigned tiles use relu(z+eps) ~ relu(z)+eps
                    # (differs by <= eps only for z in (-eps, 0)); exact
                    # max(z+eps, eps) on DVE for the rest
                    if p == 2:
                        nc.scalar.activation(kp, kpps, AF.Relu, bias=epsb)
                    else:
                        nc.vector.tensor_scalar(kp, kpps, SKP * EPS, SKP * EPS,
                                                AL.add, AL.max)
                    kps.append(kp)
                return kps

            def emit_qT(ich, p):
                l0 = ich * LCH
                qtps = work.tile([P, 512], F32, tag="work", name="qtps")[:]
                _dr_gemm(nc, qtps, whl_qk, xhls[ich], (p * P, (p + 1) * P))
                if has_qkv_b:
                    nc.scalar.activation(qt_sb[:, p, l0 : l0 + LCH], qtps,
                                         AF.Identity,
                                         bias=qkb[:, p : p + 1], scale=iqkv)
                else:
                    nc.scalar.activation(qt_sb[:, p, l0 : l0 + LCH], qtps,
                                         AF.Copy, scale=iqkv)

            def emit_kv(ich, p, kps):
                vs = ich % 2
                kvp = work.tile([P, 4, D + 1], F32, tag="work", name="kvp")[:]
                for lt in range(NSUB):
                    kp = kps[lt]
                    for h2 in range(2):
                        h = 2 * p + h2
                        for mt in range(2):
                            j = h2 * 2 + mt
                            nc.tensor.matmul(
                                kvp[:, j, :],
                                kp[:, j * P : (j + 1) * P],
                                vsb[:, vs, lt, h, :],
                                start=(lt == 0 and j == 0),
                                stop=(lt == NSUB - 1 and j == 3),
                            )
                nc.vector.tensor_tensor(
                    kv_acc[:, 4 * p : 4 * p + 4, :], kvp,
                    kv_acc[:, 4 * p : 4 * p + 4, :], AL.add)

            # ---- prep: weight DMA blocks interleaved with chunk-0/1 work ----
            def prep_w_blocks(src, nrows, dst, dst_off, scale):
                blocks = []
                nt = nrows // P
                c0 = 0
                while c0 < nt:
                    bs = min(3, nt - c0)
                    st = {}

                    def bdma(c0=c0, bs=bs, st=st):
                        wnat = wpool.tile([P, 3, DIM], F16, tag="wnat",
                                          name="wnat")[:]
                        nc.gpsimd.dma_start(
                            wnat[:, 0:bs, :],
                            src[c0 * P : (c0 + bs) * P, :].rearrange(
                                "(s p) k -> p s k", p=P))
                        st["wnat"] = wnat

                    def bcomp(c0=c0, bs=bs, st=st):
                        wnat = st["wnat"]
                        for kk in range(KT):
                            tp = work.tile([P, 512], F16, tag="work",
                                           name="ptp")[:]
                            for j in range(bs):
                                nc.tensor.transpose(
                                    tp[:, j * P : (j + 1) * P],
                                    wnat[:, j, kk * P : (kk + 1) * P], ident16)
                            hi = dst[:, kk, 0,
                                     dst_off + c0 * P : dst_off + (c0 + bs) * P]
                            nc.scalar.activation(hi, tp[:, 0 : bs * P], AF.Copy,
                                                 scale=scale)
                            nc.vector.scalar_tensor_tensor(
                                dst[:, kk, 1,
                                    dst_off + c0 * P : dst_off + (c0 + bs) * P],
                                tp[:, 0 : bs * P], scale, hi,
                                AL.mult, AL.subtract)

                    blocks.append((bdma, bcomp))
                    c0 += bs
                return blocks

            pm_st = {}

            def prep_pm_dma():
                pmn = wpool.tile([P, 2, D], F16, tag="pmn", name="pmn")[:]
                nc.gpsimd.dma_start(pmn, pm_d.rearrange("(s p) d -> p s d", p=P))
                pm_st["pmn"] = pmn

            def prep_pm():
                pmn = pm_st["pmn"]
                tp = work.tile([P, 512], F16, tag="work", name="ptp")[:]
                for s in range(2):
                    nc.tensor.transpose(tp[0:D, s * P : (s + 1) * P],
                                        pmn[:, s, :], ident16)
                nc.gpsimd.memset(pmbd, 0.0)
                nc.scalar.activation(pmbd[0:D, 0, 0:M], tp[0:D, 0:M], AF.Copy,
                                     scale=SPM * RATIO)
                nc.scalar.activation(pmbd[0:D, 1, M : 2 * M], tp[0:D, 0:M],
                                     AF.Copy, scale=SPM * RATIO)
                nc.scalar.activation(pmt2[0:D, :], tp[0:D, 0:M], AF.Copy,
                                     scale=RATIO)
                nc.scalar.activation(pmt2[D:P, :], tp[0:D, 0:M], AF.Copy,
                                     scale=RATIO)

            dma_x(0)
            wv = prep_w_blocks(qkvw_d[2 * DIM : 3 * DIM, :], DIM, wvhl, 0, SW)
            wqk_k = prep_w_blocks(qkvw_d[DIM : 2 * DIM, :], DIM, whl_qk,
                                  DIM, SW)
            wqk_q = prep_w_blocks(qkvw_d[0:DIM, :], DIM, whl_qk, 0, SW)
            wp = prep_w_blocks(projw_d, DIM, wphl, 0, SP)

            nc.gpsimd.memset(kv_acc, 0.0)
            blocks = wv + wqk_k + wqk_q + wp
            bst = {"dma": 0, "comp": 0}

            def bdma_next():
                if bst["dma"] < len(blocks):
                    blocks[bst["dma"]][0]()
                    bst["dma"] += 1

            def bcomp_next():
                if bst["comp"] < len(blocks):
                    blocks[bst["comp"]][1]()
                    bst["comp"] += 1
                    bdma_next()

            nop = lambda: None
            # phase A: wv + k-part of wqk; q-part and proj stream into chunk 0
            nA = len(wv) + len(wqk_k)
            fillers = {
                0: [lambda: transp_x(0, range(0, 3)),
                    (lambda: dma_x(1)) if 1 < NCH else nop],
                1: [lambda: transp_x(0, range(3, KT)), prep_pm],
                2: [lambda: emit_v(0, (0,)), lambda: emit_v(0, (1,)),
                    (lambda: transp_x(1, range(0, 3))) if 1 < NCH else nop],
                3: [lambda: emit_v(0, (2,)), lambda: emit_v(0, (3,)),
                    (lambda: transp_x(1, range(3, KT))) if 1 < NCH else nop,
                    (lambda: dma_x(2)) if 2 < NCH else nop],
            }
            bdma_next()
            prep_pm_dma()
            bdma_next()
            # PE p-state warmup: burn the cold DMA-wait ramping the clock
            wu = work.tile([P, 512], F16, tag="work", name="wu")[:]
            for _ in range(7):
                for s in range(4):
                    nc.tensor.transpose(wu[:, s * P : (s + 1) * P], ident16,
                                        ident16)
            for i in range(nA):
                for f in fillers.get(i, []):
                    f()
                bcomp_next()

            # ---- pass 1 main loop ----
            for ich in range(NCH):
                first = ich == 0
                pend_kt = None
                pend = None
                for p in range(NPAIR):
                    kt = emit_kT(ich, p)
                    if first:
                        # stream remaining weight blocks (q-part + proj)
                        bcomp_next()
                        if p >= 3:
                            emit_qT(0, p - 3)
                    else:
                        emit_qT(ich, p)
                    if pend is not None:
                        emit_kv(ich, pend[0], pend[1])
                        if ich == NCH - 1:
                            pp = pend[0]
                            nc.scalar.activation(
                                kvm.rearrange("p b j c -> p (b j) c")[
                                    :, 4 * pp : 4 * pp + 4, :],
                                kv_acc[:, 4 * pp : 4 * pp + 4, :],
                                AF.Copy, scale=1.0 / SKP)
                        pend = None
                    if pend_kt is not None:
                        pend = (pend_kt[0], emit_kp(pend_kt[0], pend_kt[1]))
                    pend_kt = (p, kt)
                    if ich == NCH - 1 and not first:
                        # chunk-0 q features computed here so pass 2 starts hot
                        emit_qp(0, (2 * p, 2 * p + 1), work, "work")
                    if p == 0 and 1 <= ich and ich + 1 < NCH:
                        transp_x(ich + 1, range(0, 3))
                    if p == 2 and 1 <= ich and ich + 1 < NCH:
                        transp_x(ich + 1, range(3, KT))
                    if p == 3 and 1 <= ich and ich + 2 < NCH:
                        dma_x(ich + 2)
                    if p >= 3 and ich + 1 < NCH:
                        emit_v(ich + 1, (p - 3,))
                if pend is not None:
                    emit_kv(ich, pend[0], pend[1])
                    if ich == NCH - 1:
                        pp = pend[0]
                        nc.scalar.activation(
                            kvm.rearrange("p b j c -> p (b j) c")[
                                :, 4 * pp : 4 * pp + 4, :],
                            kv_acc[:, 4 * pp : 4 * pp + 4, :],
                            AF.Copy, scale=1.0 / SKP)
                pend = (pend_kt[0], emit_kp(pend_kt[0], pend_kt[1]))
                if ich + 1 < NCH:
                    emit_v(ich + 1, (3,))
                emit_kv(ich, pend[0], pend[1])
                if ich == NCH - 1:
                    nc.scalar.activation(
                        kvm.rearrange("p b j c -> p (b j) c")[:, 20:24, :],
                        kv_acc[:, 20:24, :], AF.Copy, scale=1.0 / SKP)
                if first:
                    for p3 in range(3, NPAIR):
                        emit_qT(0, p3)
                    if NCH == 1:
                        emit_qp(0, range(H), work, "work")
                xhls.pop(ich)

    with tc.tile_pool(name="csps", bufs=1, space="PSUM") as cspool:
        css = []
        for b in range(4):
            cs = cspool.tile([1, 6, D + 1], F32, tag=f"cs{b}", name="cs")[:]
            for j in range(6):
                nc.tensor.matmul(cs[:, j, :], epscol, kvm[:, b, j, :],
                                 start=(j == 0), stop=(j == 5))
            css.append(cs)
        for b in range(4):
            nc.scalar.copy(kvmcs[:, b], css[b])

    # ---- pass 2 ----
    with tc.tile_pool(name="p2attn", bufs=2) as atp_sb, \
         tc.tile_pool(name="p2rd", bufs=2) as rdp, \
         tc.tile_pool(name="p2ahl", bufs=2) as ahlp, \
         tc.tile_pool(name="p2y", bufs=2) as yp, \
         tc.tile_pool(name="ps2qp", bufs=3, space="PSUM") as qppsum, \
         tc.tile_pool(name="ps2nm", bufs=2, space="PSUM") as numpsum, \
         tc.tile_pool(name="ps2at", bufs=1, space="PSUM") as atpsum, \
         tc.tile_pool(name="ps2y", bufs=1, space="PSUM") as ypsum:

        def emit_y(ich, ahl, lt):
            l0 = ich * LCH
            yps = ypsum.tile([P, DIM], F32, tag="yps", name="yps")[:]
            for c0, c1 in ((0, 512), (512, DIM)):
                b = (ones1, pb_row[:, c0:c1]) if has_proj_b else None
                _dr_gemm(nc, yps[:, c0:c1], wphl, ahl, (c0, c1), lt=lt, bias=b)
            ysb = yp.tile([P, DIM], F32, tag="ysb", name="ysb")[:]
            nc.scalar.activation(ysb, yps, AF.Copy, scale=iy)
            nc.sync.dma_start(y_d[l0 + lt * P : l0 + (lt + 1) * P, :], ysb)

        pend_y = []
        for ich in range(NCH):
            qp_sb = qp2[ich % 2]
            ahl = ahlp.tile([P, KT, 2, LCH], F8, tag="ahl", name="ahl")[:]
            for lt in range(NSUB):
                nmps = [numpsum.tile([P, 6, D + 1], F32, tag="nm", name="nmps")[:]
                        for _ in range(2)]
                for h in range(H):
                    g = h // 6
                    for mt in range(2):
                        ai = h * 2 + mt
                        if _QP_ENG[ai] == "act" and h % 6 != 0:
                            nc.tensor.matmul(
                                nmps[g][:, h % 6, :], ones1,
                                kvmcs[0:1, ai // 6, ai % 6, :],
                                start=False, stop=False)
                    for mt in range(2):
                        ai = h * 2 + mt
                        nc.tensor.matmul(
                            nmps[g][:, h % 6, :],
                            qp_sb[:, h, mt, lt * P : (lt + 1) * P],
                            kvm[:, ai // 6, ai % 6, :],
                            start=(mt == 0 and h % 6 == 0),
                            stop=(mt == 1 and h % 6 == 5))
                rd = rdp.tile([P, H], F32, tag="rd", name="rd")[:]
                attn = atp_sb.tile([P, H, D], F16, tag="attn", name="attn")[:]
                for g in range(2):
                    nc.vector.reciprocal(rd[:, g * 6 : (g + 1) * 6],
                                         nmps[g][:, :, D])
                    nc.vector.tensor_tensor(
                        attn[:, g * 6 : (g + 1) * 6, :],
                        nmps[g][:, :, 0:D],
                        rd[:, g * 6 : (g + 1) * 6, None].to_broadcast([P, 6, D]),
                        AL.mult)
                if ich + 1 < NCH:
                    emit_qp(ich + 1, range(3 * lt, 3 * lt + 3), qppsum, "qps")
                if len(pend_y) >= (2 if ich + 1 < NCH else 1):
                    emit_y(*pend_y.pop(0))
                atps = atpsum.tile([P, DIM], F16, tag="at", name="atps")[:]
                for kk in range(KT):
                    nc.tensor.transpose(atps[:, kk * P : (kk + 1) * P],
                                        attn[:, 2 * kk : 2 * kk + 2, :], ident16)
                a3 = atps.rearrange("p (k l) -> p k l", k=KT)
                nc.scalar.activation(ahl[:, :, 0, lt * P : (lt + 1) * P], a3,
                                     AF.Copy, scale=SA)
                nc.vector.scalar_tensor_tensor(
                    ahl[:, :, 1, lt * P : (lt + 1) * P], a3, SA,
                    ahl[:, :, 0, lt * P : (lt + 1) * P], AL.mult, AL.subtract)
                pend_y.append((ich, ahl, lt))
        for args in pend_y:
            emit_y(*args)


_CACHE = {}


def _get_nc(L=4096, hqb=False, hpb=False):
    key = ("nc", L, hqb, hpb)
    if key not in _CACHE:
        _CACHE[key] = build(L, hqb, hpb)
    return _CACHE[key]


last_exec_time_ns = None
last_profile = None


def kernel(x, qkv_w, qkv_b, proj_w, proj_b, proj_mat):
    global last_exec_time_ns, last_profile
    from concourse.bass_utils import run_bass_kernel_spmd

    x = np.asarray(x, np.float32)
    B, L, _ = x.shape
    hqb = bool(np.any(np.asarray(qkv_b)))
    hpb = bool(np.any(np.asarray(proj_b)))
    nc = _get_nc(L, hqb, hpb)
    base = {
        "qkv_w": np.ascontiguousarray(np.asarray(qkv_w, np.float32)),
        "qkv_b": np.ascontiguousarray(np.asarray(qkv_b, np.float32)),
        "proj_w": np.ascontiguousarray(np.asarray(proj_w, np.float32)),
        "proj_b": np.ascontiguousarray(np.asarray(proj_b, np.float32)),
        "proj_mat": np.ascontiguousarray(np.asarray(proj_mat, np.float32)),
    }
    in_maps = [dict(base, x=np.ascontiguousarray(x[b])) for b in range(B)]
    trace = bool(int(os.environ.get("KERNEL_TRACE", "0")))
    res = run_bass_kernel_spmd(nc, in_maps, core_ids=list(range(B)), trace=trace)
    last_exec_time_ns = res.exec_time_ns
    last_profile = res.profile_json
    return np.stack([res.results[b]["y"] for b in range(B)], axis=0)


def _ref_np(x, qkv_w, qkv_b, proj_w, proj_b, proj_mat):
    Ls = x.shape[0]
    qkv = x @ qkv_w.T + qkv_b
    qkv = qkv.reshape(Ls, 3, H, D)
    q, k, v = qkv[:, 0], qkv[:, 1], qkv[:, 2]
    qp = np.maximum(RATIO * np.einsum("lhd,md->lhm", q, proj_mat), 0) + EPS
    kp = np.maximum(RATIO * np.einsum("lhd,md->lhm", k, proj_mat), 0) + EPS
    kv = np.einsum("lhm,lhd->hmd", kp, v)
    ks = kp.sum(axis=0)
    num = np.einsum("lhm,hmd->lhd", qp, kv)
    den = np.einsum("lhm,hm->lh", qp, ks)
    out = (num / den[..., None]).reshape(Ls, DIM)
    return out @ proj_w.T + proj_b


if __name__ == "__main__":
    from concourse.bass_interp import CoreSim

    Ls = int(os.environ.get("SIM_L", "512"))
    use_bias = bool(int(os.environ.get("SIM_BIAS", "1")))
    rng = np.random.default_rng(0)
    x = rng.standard_normal((Ls, DIM), dtype=np.float32)
    qkv_w = (rng.standard_normal((3 * DIM, DIM), dtype=np.float32) * DIM**-0.5)
    qkv_b = (rng.standard_normal(3 * DIM, dtype=np.float32) * 0.1
             if use_bias else np.zeros(3 * DIM, np.float32))
    proj_w = (rng.standard_normal((DIM, DIM), dtype=np.float32) * DIM**-0.5)
    proj_b = (rng.standard_normal(DIM, dtype=np.float32) * 0.1
              if use_bias else np.zeros(DIM, np.float32))
    proj_mat = rng.standard_normal((M, D), dtype=np.float32)

    print(f"building L={Ls} bias={use_bias} ...")
    nc = build(Ls, use_bias, use_bias)
    print("simulating ...")
    sim = CoreSim(nc)
    for name, arr in [("x", x), ("qkv_w", qkv_w), ("qkv_b", qkv_b),
                      ("proj_w", proj_w), ("proj_b", proj_b),
                      ("proj_mat", proj_mat)]:
        sim.tensor(name)[:] = arr
    sim.simulate(check_with_hw=False)
    got = np.array(sim.tensor("y"))
    want = _ref_np(x, qkv_w, qkv_b, proj_w, proj_b, proj_mat)
    err = np.abs(got - want)
    rel = np.linalg.norm(got - want) / np.linalg.norm(want)
    print("max abs err:", err.max(), " rel fro err:", rel)
    assert rel < 2e-2, "sim mismatch"
    print("SIM OK")


# revision 13
# speedup vs baseline: 1.0230x; 1.0001x over previous
"""FAVOR+ (Performer) non-causal linear attention on 8 Trainium2 NeuronCores.

Sharding: data-parallel over batch B=8 -> one batch element per core.

Per-core pipeline (L=4096, DIM=768, H=12, D=64, M=256):
  prep : cast-DMA weights to fp16, PE-transpose to feature-major, split into
         fp8e4m3 hi/lo pairs (scaled) for DoubleRow matmuls; DMA blocks
         interleaved with chunk-0/1 transposes and v so PE never starves
  pass1: per 512-row chunk: cast-DMA x to fp16; PE-transpose -> xT hi/lo fp8;
         kT/qT/v via fp8 DoubleRow hi/lo-compensated GEMMs (4.5 cyc per
         128x512 output tile instead of 6); k features fp16 with blockdiag pm
         (2 heads per matmul); kv accumulated m-major [m,65] into 4 persistent
         PSUM banks across all chunks (ones-augmented v gives k_sum for
         free); qT staged in SBUF fp16 (no DRAM round trip).  Emission is
         software-pipelined: transposes/v run 2 chunks ahead; kv trails one
         pair behind its kp conversion.
  mid  : kv PSUM -> fp16 SBUF (already m-major); eps*colsum(kv) rows for the
         ACT-assigned q-feature heads
  pass2: q features m-major fp16 (computed one chunk ahead, spread across the
         l-tile loop; relu+eps on DVE/Pool, plain relu on ACT with the eps
         restored by a rank-1 matmul into num); num L-major [l,65] (den =
         col 64); reciprocal + broadcast multiply on DVE; attn PE-transposed
         to feature-major, split fp8 hi/lo; y via DoubleRow GEMM -> DMA out
"""

import math
import os
import sys
from contextlib import ExitStack

import numpy as np

for _p in ("/opt/trn_rl_repo",):
    if _p not in sys.path and os.path.isdir(_p):
        sys.path.insert(0, _p)

import concourse.bass as bass  # noqa: E402
import concourse.mybir as mybir  # noqa: E402
import concourse.tile as tile  # noqa: E402
from concourse import bacc  # noqa: E402

P = 128
DIM = 768
H = 12
D = 64
M = 256
KT = DIM // P   # 6 contraction c-tiles
NPAIR = H // 2  # 6 head pairs
EPS = 1e-3
RATIO = 1.0 / math.sqrt(float(M))

SX = 16.0    # x ~ N(0,1)
SW = 32.0    # qkv_w ~ N(0, 1/768)
SA = 64.0    # attn ~ 0.1
SP = 32.0    # proj_w ~ N(0, 1/768)
SKT = 16.0   # kT ~ N(0,1) -> fp8 for the k-feature GEMM
SPM = 32.0   # RATIO*pm ~ N(0, 1/256) -> fp8
SKP = SKT * SPM  # k-feature path runs in this scaled domain until kvm

F32 = mybir.dt.float32
F16 = mybir.dt.float16
F8 = mybir.dt.float8e4
AL = mybir.AluOpType
AF = mybir.ActivationFunctionType
DR = mybir.MatmulPerfMode.DoubleRow

# pass-2 q-feature conversion engine per (head, mtile) slot ai=h*2+mt.
# Slots of the group-starting heads (ai 0,1,12,13) stay off ACT so each num
# PSUM group's first instruction is its start matmul.
_QP_ENG = {}
_c = 0
for _ai in range(2 * H):
    if _ai in (0, 1, 12, 13):
        _QP_ENG[_ai] = "dve"
    else:
        _QP_ENG[_ai] = ("act", "act", "dve")[_c % 3]
        _c += 1


def build(L=4096, has_qkv_b=False, has_proj_b=False):
    LCH = 512
    NCH = L // LCH
    NSUB = LCH // P  # 4

    nc = bacc.Bacc("TRN2", target_bir_lowering=False, debug=False)
    x_d = nc.dram_tensor("x", [L, DIM], F32, kind="ExternalInput").ap()
    qkvw_d = nc.dram_tensor("qkv_w", [3 * DIM, DIM], F32, kind="ExternalInput").ap()
    qkvb_d = nc.dram_tensor("qkv_b", [3 * DIM], F32, kind="ExternalInput").ap()
    projw_d = nc.dram_tensor("proj_w", [DIM, DIM], F32, kind="ExternalInput").ap()
    projb_d = nc.dram_tensor("proj_b", [DIM], F32, kind="ExternalInput").ap()
    pm_d = nc.dram_tensor("proj_mat", [M, D], F32, kind="ExternalInput").ap()
    y_d = nc.dram_tensor("y", [L, DIM], F32, kind="ExternalOutput").ap()

    with tile.TileContext(nc) as tc:
        with ExitStack() as ctx:
            _body(ctx, tc, x_d, qkvw_d, qkvb_d, projw_d, projb_d, pm_d, y_d,
                  L, LCH, NCH, NSUB, has_qkv_b, has_proj_b)
    nc.compile()
    return nc


def _dr_gemm(nc, out, whl, xhl, wcols, lt=None, bias=None):
    """Accumulating fp8 DoubleRow hi/lo-compensated GEMM over 768 contraction.

    whl/xhl: [128, KT, 2, *] fp8 with hi at [:,kk,0,:], lo at [:,kk,1,:].
    lt=None : out[wc, l]; stationary = whl cols wcols, moving = xhl  (kT/qT)
    lt given: out[l, wc]; stationary = xhl l-tile cols, moving = whl (v/y)
    """
    c0, c1 = wcols
    n = c1 - c0
    if lt is None:
        for i in range(KT // 2):
            for kk in (2 * i, 2 * i + 1):
                nc.tensor.matmul(
                    out, whl[:, kk, :, c0:c1],
                    xhl[:, kk, 0:1, :].to_broadcast([P, 2, out.shape[-1]]),
                    start=(kk == 0), stop=False, perf_mode=DR,
                )
            nc.tensor.matmul(
                out, whl[:, 2 * i : 2 * i + 2, 0, c0:c1],
                xhl[:, 2 * i : 2 * i + 2, 1, :],
                start=False, stop=(bias is None and i == KT // 2 - 1),
                perf_mode=DR,
            )
    else:
        l0 = lt * P
        for i in range(KT // 2):
            for kk in (2 * i, 2 * i + 1):
                nc.tensor.matmul(
                    out, xhl[:, kk, :, l0 : l0 + P],
                    whl[:, kk, 0:1, c0:c1].to_broadcast([P, 2, n]),
                    start=(kk == 0), stop=False, perf_mode=DR,
                )
            nc.tensor.matmul(
                out, xhl[:, 2 * i : 2 * i + 2, 0, l0 : l0 + P],
                whl[:, 2 * i : 2 * i + 2, 1, c0:c1],
                start=False, stop=(bias is None and i == KT // 2 - 1),
                perf_mode=DR,
            )
    if bias is not None:
        ones_row, brow = bias
        nc.tensor.matmul(out, ones_row, brow, start=False, stop=True)


def _body(ctx, tc, x_d, qkvw_d, qkvb_d, projw_d, projb_d, pm_d, y_d,
          L, LCH, NCH, NSUB, has_qkv_b, has_proj_b):
    nc = tc.nc
    iqkv = 1.0 / (SX * SW)
    iy = 1.0 / (SA * SP)

    persist = ctx.enter_context(tc.tile_pool(name="persist", bufs=1))

    ident16 = persist.tile([P, P], F16, tag="ident16", name="ident16")[:]
    nc.gpsimd.memset(ident16, 0.0)
    nc.gpsimd.affine_select(
        out=ident16, in_=ident16, compare_op=AL.not_equal, fill=1.0,
        base=0, pattern=[[-1, P]], channel_multiplier=1,
    )
    ones1 = persist.tile([1, P], F16, tag="ones1", name="ones1")[:]
    nc.gpsimd.memset(ones1, 1.0)
    epscol = persist.tile([P, 1], F16, tag="epscol", name="epscol")[:]
    nc.gpsimd.memset(epscol, EPS)
    epsb = persist.tile([P, 1], F32, tag="epsb", name="epsb")[:]
    nc.gpsimd.memset(epsb, SKP * EPS)

    whl_qk = persist.tile([P, KT, 2, 2 * DIM], F8, tag="whl_qk", name="whl_qk")[:]
    wvhl = persist.tile([P, KT, 2, DIM], F8, tag="wvhl", name="wvhl")[:]
    wphl = persist.tile([P, KT, 2, DIM], F8, tag="wphl", name="wphl")[:]
    # folded blockdiag pm for the fp8 DoubleRow k-feature GEMM:
    # slice 0 = [RATIO*pmT | 0] (c rows 0:64), slice 1 = [0 | RATIO*pmT]
    pmbd = persist.tile([P, 2, 2 * M], F8, tag="pmbd", name="pmbd")[:]
    pmt2 = persist.tile([P, M], F16, tag="pmt2", name="pmt2")[:]
    qt_sb = persist.tile([P, NPAIR, L], F16, tag="qt_sb", name="qt_sb")[:]
    kvm = persist.tile([P, 4, 6, D + 1], F16, tag="kvm", name="kvm")[:]
    kvmcs = persist.tile([1, 4, 6, D + 1], F16, tag="kvmcs", name="kvmcs")[:]

    if has_qkv_b:
        qkb = persist.tile([P, 2 * KT], F32, tag="qkb", name="qkb")[:]
        nc.sync.dma_start(qkb, qkvb_d.rearrange("(t p) -> p t", p=P)[:, 0 : 2 * KT])
        qkbk = persist.tile([P, KT], F32, tag="qkbk", name="qkbk")[:]
        nc.scalar.activation(qkbk, qkb[:, KT : 2 * KT], AF.Copy, scale=SKT)
        vbf = persist.tile([1, DIM], F32, tag="vbf", name="vbf")[:]
        nc.sync.dma_start(vbf, qkvb_d[2 * DIM : 3 * DIM].unsqueeze(0))
        vb_row = persist.tile([1, DIM], F16, tag="vb_row", name="vb_row")[:]
        nc.scalar.activation(vb_row, vbf, AF.Copy, scale=SX * SW)
    if has_proj_b:
        pbf = persist.tile([1, DIM], F32, tag="pbf", name="pbf")[:]
        nc.sync.dma_start(pbf, projb_d.unsqueeze(0))
        pb_row = persist.tile([1, DIM], F16, tag="pb_row", name="pb_row")[:]
        nc.scalar.activation(pb_row, pbf, AF.Copy, scale=SA * SP)

    vsb = persist.tile([P, 2, NSUB, H, D + 1], F16, tag="vsb", name="vsb")[:]
    nc.gpsimd.memset(vsb[:, :, :, :, D : D + 1], 1.0)

    # kv accumulator in SBUF fp32 (indexed by ai = h*2+mt)
    kv_acc = persist.tile([P, 2 * H, D + 1], F32, tag="kv_acc", name="kv_acc")[:]

    # pass-2 q-feature tiles, double-buffered by chunk parity
    qp2 = [persist.tile([P, H, 2, LCH], F16, tag=f"qp2_{i}", name=f"qp2_{i}")[:]
           for i in range(2)]

    def emit_qp(ich, heads, psum_pool, tag):
        l0 = ich * LCH
        qp_sb = qp2[ich % 2]
        for h in heads:
            p, h2 = h // 2, h % 2
            for mt in range(2):
                qps = psum_pool.tile([P, LCH], F32, tag=tag, name="qps")[:]
                nc.tensor.matmul(
                    qps,
                    pmt2[h2 * D : (h2 + 1) * D, mt * P : (mt + 1) * P],
                    qt_sb[h2 * D : (h2 + 1) * D, p, l0 : l0 + LCH],
                    start=True, stop=True)
                eng = _QP_ENG[h * 2 + mt]
                if eng == "act":
                    # plain relu; eps restored via rank-1 kvmcs in num
                    nc.scalar.activation(qp_sb[:, h, mt, :], qps, AF.Relu)
                else:
                    nc.vector.tensor_scalar(qp_sb[:, h, mt, :], qps,
                                            EPS, EPS, AL.add, AL.max)

    if True:
        with tc.tile_pool(name="p1x", bufs=2) as xp, \
             tc.tile_pool(name="p1xhl", bufs=2) as xhlp, \
             tc.tile_pool(name="p1kt", bufs=3) as ktp, \
             tc.tile_pool(name="p1kp", bufs=8) as kpp, \
             tc.tile_pool(name="wprep", bufs=3) as wpool, \
             tc.tile_pool(name="p1work", bufs=8, space="PSUM") as work:

            xnats = {}
            xhls = {}

            def dma_x(ich):
                l0 = ich * LCH
                xnat = xp.tile([P, NSUB, DIM], F16, tag="xnat", name="xnat")[:]
                nc.gpsimd.dma_start(
                    xnat,
                    x_d[l0 : l0 + LCH, :].rearrange("(s p) k -> p s k", p=P))
                xnats[ich] = xnat

            xhl_cur = {}

            def transp_x(ich, kks=range(KT)):
                if ich in xhl_cur:
                    xnat, xhl = xhl_cur[ich]
                else:
                    xnat = xnats.pop(ich)
                    xhl = xhlp.tile([P, KT, 2, LCH], F8, tag="xhl",
                                    name="xhl")[:]
                    xhl_cur[ich] = (xnat, xhl)
                for kk in kks:
                    tp = work.tile([P, 512], F16, tag="work", name="ttp")[:]
                    for s in range(NSUB):
                        nc.tensor.transpose(
                            tp[:, s * P : (s + 1) * P],
                            xnat[:, s, kk * P : (kk + 1) * P], ident16)
                    nc.scalar.activation(xhl[:, kk, 0, :], tp, AF.Copy,
                                         scale=SX)
                    nc.vector.scalar_tensor_tensor(
                        xhl[:, kk, 1, :], tp, SX, xhl[:, kk, 0, :],
                        AL.mult, AL.subtract)
                xhls[ich] = xhl

            def emit_v(ich, subs):
                vs = ich % 2
                xhl = xhls[ich]
                for s in subs:
                    for ci in range(2):
                        c0 = ci * 384
                        vps = work.tile([P, 512], F32, tag="work", name="vps")[:]
                        bias = None
                        if has_qkv_b:
                            bias = (ones1, vb_row[:, c0 : c0 + 384])
                        _dr_gemm(nc, vps[:, 0:384], wvhl, xhl, (c0, c0 + 384),
                                 lt=s, bias=bias)
                        nc.scalar.activation(
                            vsb[:, vs, s, 6 * ci : 6 * ci + 6, 0:D],
                            vps[:, 0:384].rearrange("p (h d) -> p h d", d=D),
                            AF.Copy, scale=iqkv)

            def emit_kT(ich, p):
                ktps = work.tile([P, 512], F32, tag="work", name="ktps")[:]
                _dr_gemm(nc, ktps, whl_qk, xhls[ich],
                         (DIM + p * P, DIM + (p + 1) * P))
                # fold [128,512] -> [64, 2, 512] fp8 (scaled) for DoubleRow
                kt = ktp.tile([P, 2, LCH], F8, tag="kt", name="kt")[:]
                for h2 in range(2):
                    if has_qkv_b:
                        nc.scalar.activation(
                            kt[0:D, h2, :], ktps[h2 * D : (h2 + 1) * D, :],
                            AF.Identity,
                            bias=qkbk[h2 * D : (h2 + 1) * D, p : p + 1],
                            scale=SKT * iqkv)
                    else:
                        nc.scalar.activation(
                            kt[0:D, h2, :], ktps[h2 * D : (h2 + 1) * D, :],
                            AF.Copy, scale=SKT * iqkv)
                return kt

            def emit_kp(p, kt):
                kps = []
                for lt in range(NSUB):
                    kpps = work.tile([P, 512], F32, tag="work", name="kpps")[:]
                    nc.tensor.matmul(kpps,
                                     kt[0:D, :, lt * P : (lt + 1) * P],
                                     pmbd[0:D], start=True, stop=True,
                                     perf_mode=DR)
                    kp = kpp.tile([P, 2 * M], F16, tag="kp", name="kp")[:]
                    # k-feature path is scaled by SKP; it cancels in num/den.
                    # ACT-assigned tiles use relu(z+eps) ~ relu(z)+eps
                    # (differs by <= eps only for z in (-eps, 0)); exact
                    # max(z+eps, eps) on DVE for the rest
                    if p == 2:
                        nc.scalar.activation(kp, kpps, AF.Relu, bias=epsb)
                    else:
                        nc.vector.tensor_scalar(kp, kpps, SKP * EPS, SKP * EPS,
                                                AL.add, AL.max)
                    kps.append(kp)
                return kps

            def emit_qT(ich, p):
                l0 = ich * LCH
                qtps = work.tile([P, 512], F32, tag="work", name="qtps")[:]
                _dr_gemm(nc, qtps, whl_qk, xhls[ich], (p * P, (p + 1) * P))
                if has_qkv_b:
                    nc.scalar.activation(qt_sb[:, p, l0 : l0 + LCH], qtps,
                                         AF.Identity,
                                         bias=qkb[:, p : p + 1], scale=iqkv)
                else:
                    nc.scalar.activation(qt_sb[:, p, l0 : l0 + LCH], qtps,
                                         AF.Copy, scale=iqkv)

            def emit_kv(ich, p, kps):
                vs = ich % 2
                kvp = work.tile([P, 4, D + 1], F32, tag="work", name="kvp")[:]
                for lt in range(NSUB):
                    kp = kps[lt]
                    for h2 in range(2):
                        h = 2 * p + h2
                        for mt in range(2):
                            j = h2 * 2 + mt
                            nc.tensor.matmul(
                                kvp[:, j, :],
                                kp[:, j * P : (j + 1) * P],
                                vsb[:, vs, lt, h, :],
                                start=(lt == 0 and j == 0),
                                stop=(lt == NSUB - 1 and j == 3),
                            )
                nc.vector.tensor_tensor(
                    kv_acc[:, 4 * p : 4 * p + 4, :], kvp,
                    kv_acc[:, 4 * p : 4 * p + 4, :], AL.add)

            # ---- prep: weight DMA blocks interleaved with chunk-0/1 work ----
            def prep_w_blocks(src, nrows, dst, dst_off, scale):
                blocks = []
                nt = nrows // P
                c0 = 0
                while c0 < nt:
                    bs = min(3, nt - c0)
                    st = {}

                    def bdma(c0=c0, bs=bs, st=st):
                        wnat = wpool.tile([P, 3, DIM], F16, tag="wnat",
                                          name="wnat")[:]
                        nc.gpsimd.dma_start(
                            wnat[:, 0:bs, :],
                            src[c0 * P : (c0 + bs) * P, :].rearrange(
                                "(s p) k -> p s k", p=P))
                        st["wnat"] = wnat

                    def bcomp(c0=c0, bs=bs, st=st):
                        wnat = st["wnat"]
                        for kk in range(KT):
                            tp = work.tile([P, 512], F16, tag="work",
                                           name="ptp")[:]
                            for j in range(bs):
                                nc.tensor.transpose(
                                    tp[:, j * P : (j + 1) * P],
                                    wnat[:, j, kk * P : (kk + 1) * P], ident16)
                            hi = dst[:, kk, 0,
                                     dst_off + c0 * P : dst_off + (c0 + bs) * P]
                            nc.scalar.activation(hi, tp[:, 0 : bs * P], AF.Copy,
                                                 scale=scale)
                            nc.vector.scalar_tensor_tensor(
                                dst[:, kk, 1,
                                    dst_off + c0 * P : dst_off + (c0 + bs) * P],
                                tp[:, 0 : bs * P], scale, hi,
                                AL.mult, AL.subtract)

                    blocks.append((bdma, bcomp))
                    c0 += bs
                return blocks

            pm_st = {}

            def prep_pm_dma():
                pmn = wpool.tile([P, 2, D], F16, tag="pmn", name="pmn")[:]
                nc.gpsimd.dma_start(pmn, pm_d.rearrange("(s p) d -> p s d", p=P))
                pm_st["pmn"] = pmn

            def prep_pm():
                pmn = pm_st["pmn"]
                tp = work.tile([P, 512], F16, tag="work", name="ptp")[:]
                for s in range(2):
                    nc.tensor.transpose(tp[0:D, s * P : (s + 1) * P],
                                        pmn[:, s, :], ident16)
                nc.gpsimd.memset(pmbd, 0.0)
                nc.scalar.activation(pmbd[0:D, 0, 0:M], tp[0:D, 0:M], AF.Copy,
                                     scale=SPM * RATIO)
                nc.scalar.activation(pmbd[0:D, 1, M : 2 * M], tp[0:D, 0:M],
                                     AF.Copy, scale=SPM * RATIO)
                nc.scalar.activation(pmt2[0:D, :], tp[0:D, 0:M], AF.Copy,
                                     scale=RATIO)
                nc.scalar.activation(pmt2[D:P, :], tp[0:D, 0:M], AF.Copy,
                                     scale=RATIO)

            dma_x(0)
            wv = prep_w_blocks(qkvw_d[2 * DIM : 3 * DIM, :], DIM, wvhl, 0, SW)
            wqk_k = prep_w_blocks(qkvw_d[DIM : 2 * DIM, :], DIM, whl_qk,
                                  DIM, SW)
            wqk_q = prep_w_blocks(qkvw_d[0:DIM, :], DIM, whl_qk, 0, SW)
            wp = prep_w_blocks(projw_d, DIM, wphl, 0, SP)

            nc.gpsimd.memset(kv_acc, 0.0)
            blocks = wv + wqk_k + wqk_q + wp
            bst = {"dma": 0, "comp": 0}

            def bdma_next():
                if bst["dma"] < len(blocks):
                    blocks[bst["dma"]][0]()
                    bst["dma"] += 1

            def bcomp_next():
                if bst["comp"] < len(blocks):
                    blocks[bst["comp"]][1]()
                    bst["comp"] += 1
                    bdma_next()

            nop = lambda: None
            # phase A: wv + k-part of wqk; q-part and proj stream into chunk 0
            nA = len(wv) + len(wqk_k)
            fillers = {
                0: [lambda: transp_x(0, range(0, 3)),
                    (lambda: dma_x(1)) if 1 < NCH else nop],
                1: [lambda: transp_x(0, range(3, KT)), prep_pm],
                2: [lambda: emit_v(0, (0,)), lambda: emit_v(0, (1,)),
                    (lambda: transp_x(1, range(0, 3))) if 1 < NCH else nop],
                3: [lambda: emit_v(0, (2,)), lambda: emit_v(0, (3,)),
                    (lambda: transp_x(1, range(3, KT))) if 1 < NCH else nop,
                    (lambda: dma_x(2)) if 2 < NCH else nop],
            }
            bdma_next()
            prep_pm_dma()
            bdma_next()
            # PE p-state warmup: burn the cold DMA-wait ramping the clock
            wu = work.tile([P, 512], F16, tag="work", name="wu")[:]
            for _ in range(12):
                for s in range(4):
                    nc.tensor.transpose(wu[:, s * P : (s + 1) * P], ident16,
                                        ident16)
            for i in range(nA):
                for f in fillers.get(i, []):
                    f()
                bcomp_next()

            # ---- pass 1 main loop ----
            for ich in range(NCH):
                first = ich == 0
                pend_kt = None
                pend = None
                for p in range(NPAIR):
                    kt = emit_kT(ich, p)
                    if first:
                        # stream remaining weight blocks (q-part + proj)
                        bcomp_next()
                        if p >= 3:
                            emit_qT(0, p - 3)
                    else:
                        emit_qT(ich, p)
                    if pend is not None:
                        emit_kv(ich, pend[0], pend[1])
                        if ich == NCH - 1:
                            pp = pend[0]
                            nc.scalar.activation(
                                kvm.rearrange("p b j c -> p (b j) c")[
                                    :, 4 * pp : 4 * pp + 4, :],
                                kv_acc[:, 4 * pp : 4 * pp + 4, :],
                                AF.Copy, scale=1.0 / SKP)
                        pend = None
                    if pend_kt is not None:
                        pend = (pend_kt[0], emit_kp(pend_kt[0], pend_kt[1]))
                    pend_kt = (p, kt)
                    if ich == NCH - 1 and not first:
                        # chunk-0 q features computed here so pass 2 starts hot
                        emit_qp(0, (2 * p, 2 * p + 1), work, "work")
                    if p == 0 and 1 <= ich and ich + 1 < NCH:
                        transp_x(ich + 1, range(0, 3))
                    if p == 2 and 1 <= ich and ich + 1 < NCH:
                        transp_x(ich + 1, range(3, KT))
                    if p == 3 and 1 <= ich and ich + 2 < NCH:
                        dma_x(ich + 2)
                    if p >= 3 and ich + 1 < NCH:
                        emit_v(ich + 1, (p - 3,))
                if pend is not None:
                    emit_kv(ich, pend[0], pend[1])
                    if ich == NCH - 1:
                        pp = pend[0]
                        nc.scalar.activation(
                            kvm.rearrange("p b j c -> p (b j) c")[
                                :, 4 * pp : 4 * pp + 4, :],
                            kv_acc[:, 4 * pp : 4 * pp + 4, :],
                            AF.Copy, scale=1.0 / SKP)
                pend = (pend_kt[0], emit_kp(pend_kt[0], pend_kt[1]))
                if ich + 1 < NCH:
                    emit_v(ich + 1, (3,))
                emit_kv(ich, pend[0], pend[1])
                if ich == NCH - 1:
                    nc.scalar.activation(
                        kvm.rearrange("p b j c -> p (b j) c")[:, 20:24, :],
                        kv_acc[:, 20:24, :], AF.Copy, scale=1.0 / SKP)
                if first:
                    for p3 in range(3, NPAIR):
                        emit_qT(0, p3)
                    if NCH == 1:
                        emit_qp(0, range(H), work, "work")
                xhls.pop(ich)

    with tc.tile_pool(name="csps", bufs=1, space="PSUM") as cspool:
        css = []
        for b in range(4):
            cs = cspool.tile([1, 6, D + 1], F32, tag=f"cs{b}", name="cs")[:]
            for j in range(6):
                nc.tensor.matmul(cs[:, j, :], epscol, kvm[:, b, j, :],
                                 start=(j == 0), stop=(j == 5))
            css.append(cs)
        for b in range(4):
            nc.scalar.copy(kvmcs[:, b], css[b])

    # ---- pass 2 ----
    with tc.tile_pool(name="p2attn", bufs=2) as atp_sb, \
         tc.tile_pool(name="p2rd", bufs=2) as rdp, \
         tc.tile_pool(name="p2ahl", bufs=2) as ahlp, \
         tc.tile_pool(name="p2y", bufs=2) as yp, \
         tc.tile_pool(name="ps2qp", bufs=3, space="PSUM") as qppsum, \
         tc.tile_pool(name="ps2nm", bufs=2, space="PSUM") as numpsum, \
         tc.tile_pool(name="ps2at", bufs=1, space="PSUM") as atpsum, \
         tc.tile_pool(name="ps2y", bufs=1, space="PSUM") as ypsum:

        def emit_y(ich, ahl, lt):
            l0 = ich * LCH
            yps = ypsum.tile([P, DIM], F32, tag="yps", name="yps")[:]
            for c0, c1 in ((0, 512), (512, DIM)):
                b = (ones1, pb_row[:, c0:c1]) if has_proj_b else None
                _dr_gemm(nc, yps[:, c0:c1], wphl, ahl, (c0, c1), lt=lt, bias=b)
            ysb = yp.tile([P, DIM], F32, tag="ysb", name="ysb")[:]
            nc.scalar.activation(ysb, yps, AF.Copy, scale=iy)
            nc.sync.dma_start(y_d[l0 + lt * P : l0 + (lt + 1) * P, :], ysb)

        pend_y = []
        for ich in range(NCH):
            qp_sb = qp2[ich % 2]
            ahl = ahlp.tile([P, KT, 2, LCH], F8, tag="ahl", name="ahl")[:]
            for lt in range(NSUB):
                nmps = [numpsum.tile([P, 6, D + 1], F32, tag="nm", name="nmps")[:]
                        for _ in range(2)]
                for h in range(H):
                    g = h // 6
                    for mt in range(2):
                        ai = h * 2 + mt
                        if _QP_ENG[ai] == "act" and h % 6 != 0:
                            nc.tensor.matmul(
                                nmps[g][:, h % 6, :], ones1,
                                kvmcs[0:1, ai // 6, ai % 6, :],
                                start=False, stop=False)
                    for mt in range(2):
                        ai = h * 2 + mt
                        nc.tensor.matmul(
                            nmps[g][:, h % 6, :],
                            qp_sb[:, h, mt, lt * P : (lt + 1) * P],
                            kvm[:, ai // 6, ai % 6, :],
                            start=(mt == 0 and h % 6 == 0),
                            stop=(mt == 1 and h % 6 == 5))
                rd = rdp.tile([P, H], F32, tag="rd", name="rd")[:]
                attn = atp_sb.tile([P, H, D], F16, tag="attn", name="attn")[:]
                for g in range(2):
                    nc.vector.reciprocal(rd[:, g * 6 : (g + 1) * 6],
                                         nmps[g][:, :, D])
                    nc.vector.tensor_tensor(
                        attn[:, g * 6 : (g + 1) * 6, :],
                        nmps[g][:, :, 0:D],
                        rd[:, g * 6 : (g + 1) * 6, None].to_broadcast([P, 6, D]),
                        AL.mult)
                if ich + 1 < NCH:
                    emit_qp(ich + 1, range(3 * lt, 3 * lt + 3), qppsum, "qps")
                if len(pend_y) >= (2 if ich + 1 < NCH else 1):
                    emit_y(*pend_y.pop(0))
                atps = atpsum.tile([P, DIM], F16, tag="at", name="atps")[:]
                for kk in range(KT):
                    nc.tensor.transpose(atps[:, kk * P : (kk + 1) * P],
                                        attn[:, 2 * kk : 2 * kk + 2, :], ident16)
                a3 = atps.rearrange("p (k l) -> p k l", k=KT)
                nc.scalar.activation(ahl[:, :, 0, lt * P : (lt + 1) * P], a3,
                                     AF.Copy, scale=SA)
                nc.vector.scalar_tensor_tensor(
                    ahl[:, :, 1, lt * P : (lt + 1) * P], a3, SA,
                    ahl[:, :, 0, lt * P : (lt + 1) * P], AL.mult, AL.subtract)
                pend_y.append((ich, ahl, lt))
        for args in pend_y:
            emit_y(*args)


_CACHE = {}


def _get_nc(L=4096, hqb=False, hpb=False):
    key = ("nc", L, hqb, hpb)
    if key not in _CACHE:
        _CACHE[key] = build(L, hqb, hpb)
    return _CACHE[key]


last_exec_time_ns = None
last_profile = None


def kernel(x, qkv_w, qkv_b, proj_w, proj_b, proj_mat):
    global last_exec_time_ns, last_profile
    from concourse.bass_utils import run_bass_kernel_spmd

    x = np.asarray(x, np.float32)
    B, L, _ = x.shape
    hqb = bool(np.any(np.asarray(qkv_b)))
    hpb = bool(np.any(np.asarray(proj_b)))
    nc = _get_nc(L, hqb, hpb)
    base = {
        "qkv_w": np.ascontiguousarray(np.asarray(qkv_w, np.float32)),
        "qkv_b": np.ascontiguousarray(np.asarray(qkv_b, np.float32)),
        "proj_w": np.ascontiguousarray(np.asarray(proj_w, np.float32)),
        "proj_b": np.ascontiguousarray(np.asarray(proj_b, np.float32)),
        "proj_mat": np.ascontiguousarray(np.asarray(proj_mat, np.float32)),
    }
    in_maps = [dict(base, x=np.ascontiguousarray(x[b])) for b in range(B)]
    trace = bool(int(os.environ.get("KERNEL_TRACE", "0")))
    res = run_bass_kernel_spmd(nc, in_maps, core_ids=list(range(B)), trace=trace)
    last_exec_time_ns = res.exec_time_ns
    last_profile = res.profile_json
    return np.stack([res.results[b]["y"] for b in range(B)], axis=0)


def _ref_np(x, qkv_w, qkv_b, proj_w, proj_b, proj_mat):
    Ls = x.shape[0]
    qkv = x @ qkv_w.T + qkv_b
    qkv = qkv.reshape(Ls, 3, H, D)
    q, k, v = qkv[:, 0], qkv[:, 1], qkv[:, 2]
    qp = np.maximum(RATIO * np.einsum("lhd,md->lhm", q, proj_mat), 0) + EPS
    kp = np.maximum(RATIO * np.einsum("lhd,md->lhm", k, proj_mat), 0) + EPS
    kv = np.einsum("lhm,lhd->hmd", kp, v)
    ks = kp.sum(axis=0)
    num = np.einsum("lhm,hmd->lhd", qp, kv)
    den = np.einsum("lhm,hm->lh", qp, ks)
    out = (num / den[..., None]).reshape(Ls, DIM)
    return out @ proj_w.T + proj_b


if __name__ == "__main__":
    from concourse.bass_interp import CoreSim

    Ls = int(os.environ.get("SIM_L", "512"))
    use_bias = bool(int(os.environ.get("SIM_BIAS", "1")))
    rng = np.random.default_rng(0)
    x = rng.standard_normal((Ls, DIM), dtype=np.float32)
    qkv_w = (rng.standard_normal((3 * DIM, DIM), dtype=np.float32) * DIM**-0.5)
    qkv_b = (rng.standard_normal(3 * DIM, dtype=np.float32) * 0.1
             if use_bias else np.zeros(3 * DIM, np.float32))
    proj_w = (rng.standard_normal((DIM, DIM), dtype=np.float32) * DIM**-0.5)
    proj_b = (rng.standard_normal(DIM, dtype=np.float32) * 0.1
              if use_bias else np.zeros(DIM, np.float32))
    proj_mat = rng.standard_normal((M, D), dtype=np.float32)

    print(f"building L={Ls} bias={use_bias} ...")
    nc = build(Ls, use_bias, use_bias)
    print("simulating ...")
    sim = CoreSim(nc)
    for name, arr in [("x", x), ("qkv_w", qkv_w), ("qkv_b", qkv_b),
                      ("proj_w", proj_w), ("proj_b", proj_b),
                      ("proj_mat", proj_mat)]:
        sim.tensor(name)[:] = arr
    sim.simulate(check_with_hw=False)
    got = np.array(sim.tensor("y"))
    want = _ref_np(x, qkv_w, qkv_b, proj_w, proj_b, proj_mat)
    err = np.abs(got - want)
    rel = np.linalg.norm(got - want) / np.linalg.norm(want)
    print("max abs err:", err.max(), " rel fro err:", rel)
    assert rel < 2e-2, "sim mismatch"
    print("SIM OK")


# revision 14
# speedup vs baseline: 1.0232x; 1.0002x over previous
"""FAVOR+ (Performer) non-causal linear attention on 8 Trainium2 NeuronCores.

Sharding: data-parallel over batch B=8 -> one batch element per core.

Per-core pipeline (L=4096, DIM=768, H=12, D=64, M=256):
  prep : cast-DMA weights to fp16, PE-transpose to feature-major, split into
         fp8e4m3 hi/lo pairs (scaled) for DoubleRow matmuls; DMA blocks
         interleaved with chunk-0/1 transposes and v so PE never starves
  pass1: per 512-row chunk: cast-DMA x to fp16; PE-transpose -> xT hi/lo fp8;
         kT/qT/v via fp8 DoubleRow hi/lo-compensated GEMMs (4.5 cyc per
         128x512 output tile instead of 6); k features fp16 with blockdiag pm
         (2 heads per matmul); kv accumulated m-major [m,65] into 4 persistent
         PSUM banks across all chunks (ones-augmented v gives k_sum for
         free); qT staged in SBUF fp16 (no DRAM round trip).  Emission is
         software-pipelined: transposes/v run 2 chunks ahead; kv trails one
         pair behind its kp conversion.
  mid  : kv PSUM -> fp16 SBUF (already m-major); eps*colsum(kv) rows for the
         ACT-assigned q-feature heads
  pass2: q features m-major fp16 (computed one chunk ahead, spread across the
         l-tile loop; relu+eps on DVE/Pool, plain relu on ACT with the eps
         restored by a rank-1 matmul into num); num L-major [l,65] (den =
         col 64); reciprocal + broadcast multiply on DVE; attn PE-transposed
         to feature-major, split fp8 hi/lo; y via DoubleRow GEMM -> DMA out
"""

import math
import os
import sys
from contextlib import ExitStack

import numpy as np

for _p in ("/opt/trn_rl_repo",):
    if _p not in sys.path and os.path.isdir(_p):
        sys.path.insert(0, _p)

import concourse.bass as bass  # noqa: E402
import concourse.mybir as mybir  # noqa: E402
import concourse.tile as tile  # noqa: E402
from concourse import bacc  # noqa: E402

P = 128
DIM = 768
H = 12
D = 64
M = 256
KT = DIM // P   # 6 contraction c-tiles
NPAIR = H // 2  # 6 head pairs
EPS = 1e-3
RATIO = 1.0 / math.sqrt(float(M))

SX = 16.0    # x ~ N(0,1)
SW = 32.0    # qkv_w ~ N(0, 1/768)
SA = 64.0    # attn ~ 0.1
SP = 32.0    # proj_w ~ N(0, 1/768)
SKT = 16.0   # kT ~ N(0,1) -> fp8 for the k-feature GEMM
SPM = 32.0   # RATIO*pm ~ N(0, 1/256) -> fp8
SKP = SKT * SPM  # k-feature path runs in this scaled domain until kvm

F32 = mybir.dt.float32
F16 = mybir.dt.float16
F8 = mybir.dt.float8e4
AL = mybir.AluOpType
AF = mybir.ActivationFunctionType
DR = mybir.MatmulPerfMode.DoubleRow

# pass-2 q-feature conversion engine per (head, mtile) slot ai=h*2+mt.
# Slots of the group-starting heads (ai 0,1,12,13) stay off ACT so each num
# PSUM group's first instruction is its start matmul.
_QP_ENG = {}
_c = 0
for _ai in range(2 * H):
    if _ai in (0, 1, 12, 13):
        _QP_ENG[_ai] = "dve"
    else:
        _QP_ENG[_ai] = ("act", "act", "dve")[_c % 3]
        _c += 1


def build(L=4096, has_qkv_b=False, has_proj_b=False):
    LCH = 512
    NCH = L // LCH
    NSUB = LCH // P  # 4

    nc = bacc.Bacc("TRN2", target_bir_lowering=False, debug=False)
    x_d = nc.dram_tensor("x", [L, DIM], F32, kind="ExternalInput").ap()
    qkvw_d = nc.dram_tensor("qkv_w", [3 * DIM, DIM], F32, kind="ExternalInput").ap()
    qkvb_d = nc.dram_tensor("qkv_b", [3 * DIM], F32, kind="ExternalInput").ap()
    projw_d = nc.dram_tensor("proj_w", [DIM, DIM], F32, kind="ExternalInput").ap()
    projb_d = nc.dram_tensor("proj_b", [DIM], F32, kind="ExternalInput").ap()
    pm_d = nc.dram_tensor("proj_mat", [M, D], F32, kind="ExternalInput").ap()
    y_d = nc.dram_tensor("y", [L, DIM], F32, kind="ExternalOutput").ap()

    with tile.TileContext(nc) as tc:
        with ExitStack() as ctx:
            _body(ctx, tc, x_d, qkvw_d, qkvb_d, projw_d, projb_d, pm_d, y_d,
                  L, LCH, NCH, NSUB, has_qkv_b, has_proj_b)
    nc.compile()
    return nc


def _dr_gemm(nc, out, whl, xhl, wcols, lt=None, bias=None):
    """Accumulating fp8 DoubleRow hi/lo-compensated GEMM over 768 contraction.

    whl/xhl: [128, KT, 2, *] fp8 with hi at [:,kk,0,:], lo at [:,kk,1,:].
    lt=None : out[wc, l]; stationary = whl cols wcols, moving = xhl  (kT/qT)
    lt given: out[l, wc]; stationary = xhl l-tile cols, moving = whl (v/y)
    """
    c0, c1 = wcols
    n = c1 - c0
    if lt is None:
        for i in range(KT // 2):
            for kk in (2 * i, 2 * i + 1):
                nc.tensor.matmul(
                    out, whl[:, kk, :, c0:c1],
                    xhl[:, kk, 0:1, :].to_broadcast([P, 2, out.shape[-1]]),
                    start=(kk == 0), stop=False, perf_mode=DR,
                )
            nc.tensor.matmul(
                out, whl[:, 2 * i : 2 * i + 2, 0, c0:c1],
                xhl[:, 2 * i : 2 * i + 2, 1, :],
                start=False, stop=(bias is None and i == KT // 2 - 1),
                perf_mode=DR,
            )
    else:
        l0 = lt * P
        for i in range(KT // 2):
            for kk in (2 * i, 2 * i + 1):
                nc.tensor.matmul(
                    out, xhl[:, kk, :, l0 : l0 + P],
                    whl[:, kk, 0:1, c0:c1].to_broadcast([P, 2, n]),
                    start=(kk == 0), stop=False, perf_mode=DR,
                )
            nc.tensor.matmul(
                out, xhl[:, 2 * i : 2 * i + 2, 0, l0 : l0 + P],
                whl[:, 2 * i : 2 * i + 2, 1, c0:c1],
                start=False, stop=(bias is None and i == KT // 2 - 1),
                perf_mode=DR,
            )
    if bias is not None:
        ones_row, brow = bias
        nc.tensor.matmul(out, ones_row, brow, start=False, stop=True)


def _body(ctx, tc, x_d, qkvw_d, qkvb_d, projw_d, projb_d, pm_d, y_d,
          L, LCH, NCH, NSUB, has_qkv_b, has_proj_b):
    nc = tc.nc
    iqkv = 1.0 / (SX * SW)
    iy = 1.0 / (SA * SP)

    persist = ctx.enter_context(tc.tile_pool(name="persist", bufs=1))

    ident16 = persist.tile([P, P], F16, tag="ident16", name="ident16")[:]
    nc.gpsimd.memset(ident16, 0.0)
    nc.gpsimd.affine_select(
        out=ident16, in_=ident16, compare_op=AL.not_equal, fill=1.0,
        base=0, pattern=[[-1, P]], channel_multiplier=1,
    )
    ones1 = persist.tile([1, P], F16, tag="ones1", name="ones1")[:]
    nc.gpsimd.memset(ones1, 1.0)
    epscol = persist.tile([P, 1], F16, tag="epscol", name="epscol")[:]
    nc.gpsimd.memset(epscol, EPS)
    epsb = persist.tile([P, 1], F32, tag="epsb", name="epsb")[:]
    nc.gpsimd.memset(epsb, SKP * EPS)

    whl_qk = persist.tile([P, KT, 2, 2 * DIM], F8, tag="whl_qk", name="whl_qk")[:]
    wvhl = persist.tile([P, KT, 2, DIM], F8, tag="wvhl", name="wvhl")[:]
    wphl = persist.tile([P, KT, 2, DIM], F8, tag="wphl", name="wphl")[:]
    # folded blockdiag pm for the fp8 DoubleRow k-feature GEMM:
    # slice 0 = [RATIO*pmT | 0] (c rows 0:64), slice 1 = [0 | RATIO*pmT]
    pmbd = persist.tile([P, 2, 2 * M], F8, tag="pmbd", name="pmbd")[:]
    pmt2 = persist.tile([P, M], F16, tag="pmt2", name="pmt2")[:]
    qt_sb = persist.tile([P, NPAIR, L], F16, tag="qt_sb", name="qt_sb")[:]
    kvm = persist.tile([P, 4, 6, D + 1], F16, tag="kvm", name="kvm")[:]
    kvmcs = persist.tile([1, 4, 6, D + 1], F16, tag="kvmcs", name="kvmcs")[:]

    if has_qkv_b:
        qkb = persist.tile([P, 2 * KT], F32, tag="qkb", name="qkb")[:]
        nc.sync.dma_start(qkb, qkvb_d.rearrange("(t p) -> p t", p=P)[:, 0 : 2 * KT])
        qkbk = persist.tile([P, KT], F32, tag="qkbk", name="qkbk")[:]
        nc.scalar.activation(qkbk, qkb[:, KT : 2 * KT], AF.Copy, scale=SKT)
        vbf = persist.tile([1, DIM], F32, tag="vbf", name="vbf")[:]
        nc.sync.dma_start(vbf, qkvb_d[2 * DIM : 3 * DIM].unsqueeze(0))
        vb_row = persist.tile([1, DIM], F16, tag="vb_row", name="vb_row")[:]
        nc.scalar.activation(vb_row, vbf, AF.Copy, scale=SX * SW)
    if has_proj_b:
        pbf = persist.tile([1, DIM], F32, tag="pbf", name="pbf")[:]
        nc.sync.dma_start(pbf, projb_d.unsqueeze(0))
        pb_row = persist.tile([1, DIM], F16, tag="pb_row", name="pb_row")[:]
        nc.scalar.activation(pb_row, pbf, AF.Copy, scale=SA * SP)

    vsb = persist.tile([P, 2, NSUB, H, D + 1], F16, tag="vsb", name="vsb")[:]
    nc.gpsimd.memset(vsb[:, :, :, :, D : D + 1], 1.0)

    # kv accumulator in SBUF fp32 (indexed by ai = h*2+mt)
    kv_acc = persist.tile([P, 2 * H, D + 1], F32, tag="kv_acc", name="kv_acc")[:]

    # pass-2 q-feature tiles, double-buffered by chunk parity
    qp2 = [persist.tile([P, H, 2, LCH], F16, tag=f"qp2_{i}", name=f"qp2_{i}")[:]
           for i in range(2)]

    def emit_qp(ich, heads, psum_pool, tag):
        l0 = ich * LCH
        qp_sb = qp2[ich % 2]
        for h in heads:
            p, h2 = h // 2, h % 2
            for mt in range(2):
                qps = psum_pool.tile([P, LCH], F32, tag=tag, name="qps")[:]
                nc.tensor.matmul(
                    qps,
                    pmt2[h2 * D : (h2 + 1) * D, mt * P : (mt + 1) * P],
                    qt_sb[h2 * D : (h2 + 1) * D, p, l0 : l0 + LCH],
                    start=True, stop=True)
                eng = _QP_ENG[h * 2 + mt]
                if eng == "act":
                    # plain relu; eps restored via rank-1 kvmcs in num
                    nc.scalar.activation(qp_sb[:, h, mt, :], qps, AF.Relu)
                else:
                    nc.vector.tensor_scalar(qp_sb[:, h, mt, :], qps,
                                            EPS, EPS, AL.add, AL.max)

    if True:
        with tc.tile_pool(name="p1x", bufs=2) as xp, \
             tc.tile_pool(name="p1xhl", bufs=2) as xhlp, \
             tc.tile_pool(name="p1kt", bufs=3) as ktp, \
             tc.tile_pool(name="p1kp", bufs=8) as kpp, \
             tc.tile_pool(name="wprep", bufs=3) as wpool, \
             tc.tile_pool(name="p1work", bufs=8, space="PSUM") as work:

            xnats = {}
            xhls = {}

            def dma_x(ich):
                l0 = ich * LCH
                xnat = xp.tile([P, NSUB, DIM], F16, tag="xnat", name="xnat")[:]
                nc.gpsimd.dma_start(
                    xnat,
                    x_d[l0 : l0 + LCH, :].rearrange("(s p) k -> p s k", p=P))
                xnats[ich] = xnat

            xhl_cur = {}

            def transp_x(ich, kks=range(KT)):
                if ich in xhl_cur:
                    xnat, xhl = xhl_cur[ich]
                else:
                    xnat = xnats.pop(ich)
                    xhl = xhlp.tile([P, KT, 2, LCH], F8, tag="xhl",
                                    name="xhl")[:]
                    xhl_cur[ich] = (xnat, xhl)
                for kk in kks:
                    tp = work.tile([P, 512], F16, tag="work", name="ttp")[:]
                    for s in range(NSUB):
                        nc.tensor.transpose(
                            tp[:, s * P : (s + 1) * P],
                            xnat[:, s, kk * P : (kk + 1) * P], ident16)
                    nc.scalar.activation(xhl[:, kk, 0, :], tp, AF.Copy,
                                         scale=SX)
                    nc.vector.scalar_tensor_tensor(
                        xhl[:, kk, 1, :], tp, SX, xhl[:, kk, 0, :],
                        AL.mult, AL.subtract)
                xhls[ich] = xhl

            def emit_v(ich, subs):
                vs = ich % 2
                xhl = xhls[ich]
                for s in subs:
                    for ci in range(2):
                        c0 = ci * 384
                        vps = work.tile([P, 512], F32, tag="work", name="vps")[:]
                        bias = None
                        if has_qkv_b:
                            bias = (ones1, vb_row[:, c0 : c0 + 384])
                        _dr_gemm(nc, vps[:, 0:384], wvhl, xhl, (c0, c0 + 384),
                                 lt=s, bias=bias)
                        nc.scalar.activation(
                            vsb[:, vs, s, 6 * ci : 6 * ci + 6, 0:D],
                            vps[:, 0:384].rearrange("p (h d) -> p h d", d=D),
                            AF.Copy, scale=iqkv)

            def emit_kT(ich, p):
                ktps = work.tile([P, 512], F32, tag="work", name="ktps")[:]
                _dr_gemm(nc, ktps, whl_qk, xhls[ich],
                         (DIM + p * P, DIM + (p + 1) * P))
                # fold [128,512] -> [64, 2, 512] fp8 (scaled) for DoubleRow
                kt = ktp.tile([P, 2, LCH], F8, tag="kt", name="kt")[:]
                for h2 in range(2):
                    if has_qkv_b:
                        nc.scalar.activation(
                            kt[0:D, h2, :], ktps[h2 * D : (h2 + 1) * D, :],
                            AF.Identity,
                            bias=qkbk[h2 * D : (h2 + 1) * D, p : p + 1],
                            scale=SKT * iqkv)
                    else:
                        nc.scalar.activation(
                            kt[0:D, h2, :], ktps[h2 * D : (h2 + 1) * D, :],
                            AF.Copy, scale=SKT * iqkv)
                return kt

            def emit_kp(p, kt):
                kps = []
                for lt in range(NSUB):
                    kpps = work.tile([P, 512], F32, tag="work", name="kpps")[:]
                    nc.tensor.matmul(kpps,
                                     kt[0:D, :, lt * P : (lt + 1) * P],
                                     pmbd[0:D], start=True, stop=True,
                                     perf_mode=DR)
                    kp = kpp.tile([P, 2 * M], F16, tag="kp", name="kp")[:]
                    # k-feature path is scaled by SKP; it cancels in num/den.
                    # ACT-assigned tiles use relu(z+eps) ~ relu(z)+eps
                    # (differs by <= eps only for z in (-eps, 0)); exact
                    # max(z+eps, eps) on DVE for the rest
                    if p == 3:
                        nc.scalar.activation(kp, kpps, AF.Relu, bias=epsb)
                    else:
                        nc.vector.tensor_scalar(kp, kpps, SKP * EPS, SKP * EPS,
                                                AL.add, AL.max)
                    kps.append(kp)
                return kps

            def emit_qT(ich, p):
                l0 = ich * LCH
                qtps = work.tile([P, 512], F32, tag="work", name="qtps")[:]
                _dr_gemm(nc, qtps, whl_qk, xhls[ich], (p * P, (p + 1) * P))
                if has_qkv_b:
                    nc.scalar.activation(qt_sb[:, p, l0 : l0 + LCH], qtps,
                                         AF.Identity,
                                         bias=qkb[:, p : p + 1], scale=iqkv)
                else:
                    nc.scalar.activation(qt_sb[:, p, l0 : l0 + LCH], qtps,
                                         AF.Copy, scale=iqkv)

            def emit_kv(ich, p, kps):
                vs = ich % 2
                kvp = work.tile([P, 4, D + 1], F32, tag="work", name="kvp")[:]
                for lt in range(NSUB):
                    kp = kps[lt]
                    for h2 in range(2):
                        h = 2 * p + h2
                        for mt in range(2):
                            j = h2 * 2 + mt
                            nc.tensor.matmul(
                                kvp[:, j, :],
                                kp[:, j * P : (j + 1) * P],
                                vsb[:, vs, lt, h, :],
                                start=(lt == 0 and j == 0),
                                stop=(lt == NSUB - 1 and j == 3),
                            )
                nc.vector.tensor_tensor(
                    kv_acc[:, 4 * p : 4 * p + 4, :], kvp,
                    kv_acc[:, 4 * p : 4 * p + 4, :], AL.add)

            # ---- prep: weight DMA blocks interleaved with chunk-0/1 work ----
            def prep_w_blocks(src, nrows, dst, dst_off, scale):
                blocks = []
                nt = nrows // P
                c0 = 0
                while c0 < nt:
                    bs = min(3, nt - c0)
                    st = {}

                    def bdma(c0=c0, bs=bs, st=st):
                        wnat = wpool.tile([P, 3, DIM], F16, tag="wnat",
                                          name="wnat")[:]
                        nc.gpsimd.dma_start(
                            wnat[:, 0:bs, :],
                            src[c0 * P : (c0 + bs) * P, :].rearrange(
                                "(s p) k -> p s k", p=P))
                        st["wnat"] = wnat

                    def bcomp(c0=c0, bs=bs, st=st):
                        wnat = st["wnat"]
                        for kk in range(KT):
                            tp = work.tile([P, 512], F16, tag="work",
                                           name="ptp")[:]
                            for j in range(bs):
                                nc.tensor.transpose(
                                    tp[:, j * P : (j + 1) * P],
                                    wnat[:, j, kk * P : (kk + 1) * P], ident16)
                            hi = dst[:, kk, 0,
                                     dst_off + c0 * P : dst_off + (c0 + bs) * P]
                            nc.scalar.activation(hi, tp[:, 0 : bs * P], AF.Copy,
                                                 scale=scale)
                            nc.vector.scalar_tensor_tensor(
                                dst[:, kk, 1,
                                    dst_off + c0 * P : dst_off + (c0 + bs) * P],
                                tp[:, 0 : bs * P], scale, hi,
                                AL.mult, AL.subtract)

                    blocks.append((bdma, bcomp))
                    c0 += bs
                return blocks

            pm_st = {}

            def prep_pm_dma():
                pmn = wpool.tile([P, 2, D], F16, tag="pmn", name="pmn")[:]
                nc.gpsimd.dma_start(pmn, pm_d.rearrange("(s p) d -> p s d", p=P))
                pm_st["pmn"] = pmn

            def prep_pm():
                pmn = pm_st["pmn"]
                tp = work.tile([P, 512], F16, tag="work", name="ptp")[:]
                for s in range(2):
                    nc.tensor.transpose(tp[0:D, s * P : (s + 1) * P],
                                        pmn[:, s, :], ident16)
                nc.gpsimd.memset(pmbd, 0.0)
                nc.scalar.activation(pmbd[0:D, 0, 0:M], tp[0:D, 0:M], AF.Copy,
                                     scale=SPM * RATIO)
                nc.scalar.activation(pmbd[0:D, 1, M : 2 * M], tp[0:D, 0:M],
                                     AF.Copy, scale=SPM * RATIO)
                nc.scalar.activation(pmt2[0:D, :], tp[0:D, 0:M], AF.Copy,
                                     scale=RATIO)
                nc.scalar.activation(pmt2[D:P, :], tp[0:D, 0:M], AF.Copy,
                                     scale=RATIO)

            dma_x(0)
            wv = prep_w_blocks(qkvw_d[2 * DIM : 3 * DIM, :], DIM, wvhl, 0, SW)
            wqk_k = prep_w_blocks(qkvw_d[DIM : 2 * DIM, :], DIM, whl_qk,
                                  DIM, SW)
            wqk_q = prep_w_blocks(qkvw_d[0:DIM, :], DIM, whl_qk, 0, SW)
            wp = prep_w_blocks(projw_d, DIM, wphl, 0, SP)

            nc.gpsimd.memset(kv_acc, 0.0)
            blocks = wv + wqk_k + wqk_q + wp
            bst = {"dma": 0, "comp": 0}

            def bdma_next():
                if bst["dma"] < len(blocks):
                    blocks[bst["dma"]][0]()
                    bst["dma"] += 1

            def bcomp_next():
                if bst["comp"] < len(blocks):
                    blocks[bst["comp"]][1]()
                    bst["comp"] += 1
                    bdma_next()

            nop = lambda: None
            # phase A: wv + k-part of wqk; q-part and proj stream into chunk 0
            nA = len(wv) + len(wqk_k)
            fillers = {
                0: [lambda: transp_x(0, range(0, 3)),
                    (lambda: dma_x(1)) if 1 < NCH else nop],
                1: [lambda: transp_x(0, range(3, KT)), prep_pm],
                2: [lambda: emit_v(0, (0,)), lambda: emit_v(0, (1,)),
                    (lambda: transp_x(1, range(0, 3))) if 1 < NCH else nop],
                3: [lambda: emit_v(0, (2,)), lambda: emit_v(0, (3,)),
                    (lambda: transp_x(1, range(3, KT))) if 1 < NCH else nop,
                    (lambda: dma_x(2)) if 2 < NCH else nop],
            }
            bdma_next()
            prep_pm_dma()
            bdma_next()
            # PE p-state warmup: burn the cold DMA-wait ramping the clock
            wu = work.tile([P, 512], F16, tag="work", name="wu")[:]
            for _ in range(12):
                for s in range(4):
                    nc.tensor.transpose(wu[:, s * P : (s + 1) * P], ident16,
                                        ident16)
            for i in range(nA):
                for f in fillers.get(i, []):
                    f()
                bcomp_next()

            # ---- pass 1 main loop ----
            for ich in range(NCH):
                first = ich == 0
                pend_kt = None
                pend = None
                for p in range(NPAIR):
                    kt = emit_kT(ich, p)
                    if first:
                        # stream remaining weight blocks (q-part + proj)
                        bcomp_next()
                        if p >= 3:
                            emit_qT(0, p - 3)
                    else:
                        emit_qT(ich, p)
                    if pend is not None:
                        emit_kv(ich, pend[0], pend[1])
                        if ich == NCH - 1:
                            pp = pend[0]
                            nc.scalar.activation(
                                kvm.rearrange("p b j c -> p (b j) c")[
                                    :, 4 * pp : 4 * pp + 4, :],
                                kv_acc[:, 4 * pp : 4 * pp + 4, :],
                                AF.Copy, scale=1.0 / SKP)
                        pend = None
                    if pend_kt is not None:
                        pend = (pend_kt[0], emit_kp(pend_kt[0], pend_kt[1]))
                    pend_kt = (p, kt)
                    if ich == NCH - 1 and not first:
                        # chunk-0 q features computed here so pass 2 starts hot
                        emit_qp(0, (2 * p, 2 * p + 1), work, "work")
                    if p == 0 and 1 <= ich and ich + 1 < NCH:
                        transp_x(ich + 1, range(0, 3))
                    if p == 2 and 1 <= ich and ich + 1 < NCH:
                        transp_x(ich + 1, range(3, KT))
                    if p == 3 and 1 <= ich and ich + 2 < NCH:
                        dma_x(ich + 2)
                    if p >= 3 and ich + 1 < NCH:
                        emit_v(ich + 1, (p - 3,))
                if pend is not None:
                    emit_kv(ich, pend[0], pend[1])
                    if ich == NCH - 1:
                        pp = pend[0]
                        nc.scalar.activation(
                            kvm.rearrange("p b j c -> p (b j) c")[
                                :, 4 * pp : 4 * pp + 4, :],
                            kv_acc[:, 4 * pp : 4 * pp + 4, :],
                            AF.Copy, scale=1.0 / SKP)
                pend = (pend_kt[0], emit_kp(pend_kt[0], pend_kt[1]))
                if ich + 1 < NCH:
                    emit_v(ich + 1, (3,))
                emit_kv(ich, pend[0], pend[1])
                if ich == NCH - 1:
                    nc.scalar.activation(
                        kvm.rearrange("p b j c -> p (b j) c")[:, 20:24, :],
                        kv_acc[:, 20:24, :], AF.Copy, scale=1.0 / SKP)
                if first:
                    for p3 in range(3, NPAIR):
                        emit_qT(0, p3)
                    if NCH == 1:
                        emit_qp(0, range(H), work, "work")
                xhls.pop(ich)

    with tc.tile_pool(name="csps", bufs=1, space="PSUM") as cspool:
        css = []
        for b in range(4):
            cs = cspool.tile([1, 6, D + 1], F32, tag=f"cs{b}", name="cs")[:]
            for j in range(6):
                nc.tensor.matmul(cs[:, j, :], epscol, kvm[:, b, j, :],
                                 start=(j == 0), stop=(j == 5))
            css.append(cs)
        for b in range(4):
            nc.scalar.copy(kvmcs[:, b], css[b])

    # ---- pass 2 ----
    with tc.tile_pool(name="p2attn", bufs=2) as atp_sb, \
         tc.tile_pool(name="p2rd", bufs=2) as rdp, \
         tc.tile_pool(name="p2ahl", bufs=2) as ahlp, \
         tc.tile_pool(name="p2y", bufs=2) as yp, \
         tc.tile_pool(name="ps2qp", bufs=3, space="PSUM") as qppsum, \
         tc.tile_pool(name="ps2nm", bufs=2, space="PSUM") as numpsum, \
         tc.tile_pool(name="ps2at", bufs=1, space="PSUM") as atpsum, \
         tc.tile_pool(name="ps2y", bufs=1, space="PSUM") as ypsum:

        def emit_y(ich, ahl, lt):
            l0 = ich * LCH
            yps = ypsum.tile([P, DIM], F32, tag="yps", name="yps")[:]
            for c0, c1 in ((0, 512), (512, DIM)):
                b = (ones1, pb_row[:, c0:c1]) if has_proj_b else None
                _dr_gemm(nc, yps[:, c0:c1], wphl, ahl, (c0, c1), lt=lt, bias=b)
            ysb = yp.tile([P, DIM], F32, tag="ysb", name="ysb")[:]
            nc.scalar.activation(ysb, yps, AF.Copy, scale=iy)
            nc.sync.dma_start(y_d[l0 + lt * P : l0 + (lt + 1) * P, :], ysb)

        pend_y = []
        for ich in range(NCH):
            qp_sb = qp2[ich % 2]
            ahl = ahlp.tile([P, KT, 2, LCH], F8, tag="ahl", name="ahl")[:]
            for lt in range(NSUB):
                nmps = [numpsum.tile([P, 6, D + 1], F32, tag="nm", name="nmps")[:]
                        for _ in range(2)]
                for h in range(H):
                    g = h // 6
                    for mt in range(2):
                        ai = h * 2 + mt
                        if _QP_ENG[ai] == "act" and h % 6 != 0:
                            nc.tensor.matmul(
                                nmps[g][:, h % 6, :], ones1,
                                kvmcs[0:1, ai // 6, ai % 6, :],
                                start=False, stop=False)
                    for mt in range(2):
                        ai = h * 2 + mt
                        nc.tensor.matmul(
                            nmps[g][:, h % 6, :],
                            qp_sb[:, h, mt, lt * P : (lt + 1) * P],
                            kvm[:, ai // 6, ai % 6, :],
                            start=(mt == 0 and h % 6 == 0),
                            stop=(mt == 1 and h % 6 == 5))
                rd = rdp.tile([P, H], F32, tag="rd", name="rd")[:]
                attn = atp_sb.tile([P, H, D], F16, tag="attn", name="attn")[:]
                for g in range(2):
                    nc.vector.reciprocal(rd[:, g * 6 : (g + 1) * 6],
                                         nmps[g][:, :, D])
                    nc.vector.tensor_tensor(
                        attn[:, g * 6 : (g + 1) * 6, :],
                        nmps[g][:, :, 0:D],
                        rd[:, g * 6 : (g + 1) * 6, None].to_broadcast([P, 6, D]),
                        AL.mult)
                if ich + 1 < NCH:
                    emit_qp(ich + 1, range(3 * lt, 3 * lt + 3), qppsum, "qps")
                if len(pend_y) >= (2 if ich + 1 < NCH else 1):
                    emit_y(*pend_y.pop(0))
                atps = atpsum.tile([P, DIM], F16, tag="at", name="atps")[:]
                for kk in range(KT):
                    nc.tensor.transpose(atps[:, kk * P : (kk + 1) * P],
                                        attn[:, 2 * kk : 2 * kk + 2, :], ident16)
                a3 = atps.rearrange("p (k l) -> p k l", k=KT)
                nc.scalar.activation(ahl[:, :, 0, lt * P : (lt + 1) * P], a3,
                                     AF.Copy, scale=SA)
                nc.vector.scalar_tensor_tensor(
                    ahl[:, :, 1, lt * P : (lt + 1) * P], a3, SA,
                    ahl[:, :, 0, lt * P : (lt + 1) * P], AL.mult, AL.subtract)
                pend_y.append((ich, ahl, lt))
        for args in pend_y:
            emit_y(*args)


_CACHE = {}


def _get_nc(L=4096, hqb=False, hpb=False):
    key = ("nc", L, hqb, hpb)
    if key not in _CACHE:
        _CACHE[key] = build(L, hqb, hpb)
    return _CACHE[key]


last_exec_time_ns = None
last_profile = None


def kernel(x, qkv_w, qkv_b, proj_w, proj_b, proj_mat):
    global last_exec_time_ns, last_profile
    from concourse.bass_utils import run_bass_kernel_spmd

    x = np.asarray(x, np.float32)
    B, L, _ = x.shape
    hqb = bool(np.any(np.asarray(qkv_b)))
    hpb = bool(np.any(np.asarray(proj_b)))
    nc = _get_nc(L, hqb, hpb)
    base = {
        "qkv_w": np.ascontiguousarray(np.asarray(qkv_w, np.float32)),
        "qkv_b": np.ascontiguousarray(np.asarray(qkv_b, np.float32)),
        "proj_w": np.ascontiguousarray(np.asarray(proj_w, np.float32)),
        "proj_b": np.ascontiguousarray(np.asarray(proj_b, np.float32)),
        "proj_mat": np.ascontiguousarray(np.asarray(proj_mat, np.float32)),
    }
    in_maps = [dict(base, x=np.ascontiguousarray(x[b])) for b in range(B)]
    trace = bool(int(os.environ.get("KERNEL_TRACE", "0")))
    res = run_bass_kernel_spmd(nc, in_maps, core_ids=list(range(B)), trace=trace)
    last_exec_time_ns = res.exec_time_ns
    last_profile = res.profile_json
    return np.stack([res.results[b]["y"] for b in range(B)], axis=0)


def _ref_np(x, qkv_w, qkv_b, proj_w, proj_b, proj_mat):
    Ls = x.shape[0]
    qkv = x @ qkv_w.T + qkv_b
    qkv = qkv.reshape(Ls, 3, H, D)
    q, k, v = qkv[:, 0], qkv[:, 1], qkv[:, 2]
    qp = np.maximum(RATIO * np.einsum("lhd,md->lhm", q, proj_mat), 0) + EPS
    kp = np.maximum(RATIO * np.einsum("lhd,md->lhm", k, proj_mat), 0) + EPS
    kv = np.einsum("lhm,lhd->hmd", kp, v)
    ks = kp.sum(axis=0)
    num = np.einsum("lhm,hmd->lhd", qp, kv)
    den = np.einsum("lhm,hm->lh", qp, ks)
    out = (num / den[..., None]).reshape(Ls, DIM)
    return out @ proj_w.T + proj_b


if __name__ == "__main__":
    from concourse.bass_interp import CoreSim

    Ls = int(os.environ.get("SIM_L", "512"))
    use_bias = bool(int(os.environ.get("SIM_BIAS", "1")))
    rng = np.random.default_rng(0)
    x = rng.standard_normal((Ls, DIM), dtype=np.float32)
    qkv_w = (rng.standard_normal((3 * DIM, DIM), dtype=np.float32) * DIM**-0.5)
    qkv_b = (rng.standard_normal(3 * DIM, dtype=np.float32) * 0.1
             if use_bias else np.zeros(3 * DIM, np.float32))
    proj_w = (rng.standard_normal((DIM, DIM), dtype=np.float32) * DIM**-0.5)
    proj_b = (rng.standard_normal(DIM, dtype=np.float32) * 0.1
              if use_bias else np.zeros(DIM, np.float32))
    proj_mat = rng.standard_normal((M, D), dtype=np.float32)

    print(f"building L={Ls} bias={use_bias} ...")
    nc = build(Ls, use_bias, use_bias)
    print("simulating ...")
    sim = CoreSim(nc)
    for name, arr in [("x", x), ("qkv_w", qkv_w), ("qkv_b", qkv_b),
                      ("proj_w", proj_w), ("proj_b", proj_b),
                      ("proj_mat", proj_mat)]:
        sim.tensor(name)[:] = arr
    sim.simulate(check_with_hw=False)
    got = np.array(sim.tensor("y"))
    want = _ref_np(x, qkv_w, qkv_b, proj_w, proj_b, proj_mat)
    err = np.abs(got - want)
    rel = np.linalg.norm(got - want) / np.linalg.norm(want)
    print("max abs err:", err.max(), " rel fro err:", rel)
    assert rel < 2e-2, "sim mismatch"
    print("SIM OK")


# revision 15
# speedup vs baseline: 1.0252x; 1.0020x over previous
"""FAVOR+ (Performer) non-causal linear attention on 8 Trainium2 NeuronCores.

Sharding: data-parallel over batch B=8 -> one batch element per core.

Per-core pipeline (L=4096, DIM=768, H=12, D=64, M=256):
  prep : cast-DMA weights to fp16, PE-transpose to feature-major, split into
         fp8e4m3 hi/lo pairs (scaled) for DoubleRow matmuls; DMA blocks
         interleaved with chunk-0/1 transposes and v so PE never starves
  pass1: per 512-row chunk: cast-DMA x to fp16; PE-transpose -> xT hi/lo fp8;
         kT/qT/v via fp8 DoubleRow hi/lo-compensated GEMMs (4.5 cyc per
         128x512 output tile instead of 6); k features fp16 with blockdiag pm
         (2 heads per matmul); kv accumulated m-major [m,65] into 4 persistent
         PSUM banks across all chunks (ones-augmented v gives k_sum for
         free); qT staged in SBUF fp16 (no DRAM round trip).  Emission is
         software-pipelined: transposes/v run 2 chunks ahead; kv trails one
         pair behind its kp conversion.
  mid  : kv PSUM -> fp16 SBUF (already m-major); eps*colsum(kv) rows for the
         ACT-assigned q-feature heads
  pass2: q features m-major fp16 (computed one chunk ahead, spread across the
         l-tile loop; relu+eps on DVE/Pool, plain relu on ACT with the eps
         restored by a rank-1 matmul into num); num L-major [l,65] (den =
         col 64); reciprocal + broadcast multiply on DVE; attn PE-transposed
         to feature-major, split fp8 hi/lo; y via DoubleRow GEMM -> DMA out
"""

import math
import os
import sys
from contextlib import ExitStack

import numpy as np

for _p in ("/opt/trn_rl_repo",):
    if _p not in sys.path and os.path.isdir(_p):
        sys.path.insert(0, _p)

import concourse.bass as bass  # noqa: E402
import concourse.mybir as mybir  # noqa: E402
import concourse.tile as tile  # noqa: E402
from concourse import bacc  # noqa: E402

P = 128
DIM = 768
H = 12
D = 64
M = 256
KT = DIM // P   # 6 contraction c-tiles
NPAIR = H // 2  # 6 head pairs
EPS = 1e-3
RATIO = 1.0 / math.sqrt(float(M))

SX = 16.0    # x ~ N(0,1)
SW = 32.0    # qkv_w ~ N(0, 1/768)
SA = 64.0    # attn ~ 0.1
SP = 32.0    # proj_w ~ N(0, 1/768)
SKT = 16.0   # kT ~ N(0,1) -> fp8 for the k-feature GEMM
SPM = 32.0   # RATIO*pm ~ N(0, 1/256) -> fp8
SKP = SKT * SPM  # k-feature path runs in this scaled domain until kvm

F32 = mybir.dt.float32
F16 = mybir.dt.float16
F8 = mybir.dt.float8e4
AL = mybir.AluOpType
AF = mybir.ActivationFunctionType
DR = mybir.MatmulPerfMode.DoubleRow

# pass-2 q-feature conversion engine per (head, mtile) slot ai=h*2+mt.
# Slots of the group-starting heads (ai 0,1,12,13) stay off ACT so each num
# PSUM group's first instruction is its start matmul.
_QP_ENG = {}
_c = 0
for _ai in range(2 * H):
    if _ai in (0, 1, 12, 13):
        _QP_ENG[_ai] = "dve"
    else:
        _QP_ENG[_ai] = ("dve", "act", "act")[_c % 3]
        _c += 1


def build(L=4096, has_qkv_b=False, has_proj_b=False):
    LCH = 512
    NCH = L // LCH
    NSUB = LCH // P  # 4

    nc = bacc.Bacc("TRN2", target_bir_lowering=False, debug=False)
    x_d = nc.dram_tensor("x", [L, DIM], F32, kind="ExternalInput").ap()
    qkvw_d = nc.dram_tensor("qkv_w", [3 * DIM, DIM], F32, kind="ExternalInput").ap()
    qkvb_d = nc.dram_tensor("qkv_b", [3 * DIM], F32, kind="ExternalInput").ap()
    projw_d = nc.dram_tensor("proj_w", [DIM, DIM], F32, kind="ExternalInput").ap()
    projb_d = nc.dram_tensor("proj_b", [DIM], F32, kind="ExternalInput").ap()
    pm_d = nc.dram_tensor("proj_mat", [M, D], F32, kind="ExternalInput").ap()
    y_d = nc.dram_tensor("y", [L, DIM], F32, kind="ExternalOutput").ap()

    with tile.TileContext(nc) as tc:
        with ExitStack() as ctx:
            _body(ctx, tc, x_d, qkvw_d, qkvb_d, projw_d, projb_d, pm_d, y_d,
                  L, LCH, NCH, NSUB, has_qkv_b, has_proj_b)
    nc.compile()
    return nc


def _dr_gemm(nc, out, whl, xhl, wcols, lt=None, bias=None):
    """Accumulating fp8 DoubleRow hi/lo-compensated GEMM over 768 contraction.

    whl/xhl: [128, KT, 2, *] fp8 with hi at [:,kk,0,:], lo at [:,kk,1,:].
    lt=None : out[wc, l]; stationary = whl cols wcols, moving = xhl  (kT/qT)
    lt given: out[l, wc]; stationary = xhl l-tile cols, moving = whl (v/y)
    """
    c0, c1 = wcols
    n = c1 - c0
    if lt is None:
        for i in range(KT // 2):
            for kk in (2 * i, 2 * i + 1):
                nc.tensor.matmul(
                    out, whl[:, kk, :, c0:c1],
                    xhl[:, kk, 0:1, :].to_broadcast([P, 2, out.shape[-1]]),
                    start=(kk == 0), stop=False, perf_mode=DR,
                )
            nc.tensor.matmul(
                out, whl[:, 2 * i : 2 * i + 2, 0, c0:c1],
                xhl[:, 2 * i : 2 * i + 2, 1, :],
                start=False, stop=(bias is None and i == KT // 2 - 1),
                perf_mode=DR,
            )
    else:
        l0 = lt * P
        for i in range(KT // 2):
            for kk in (2 * i, 2 * i + 1):
                nc.tensor.matmul(
                    out, xhl[:, kk, :, l0 : l0 + P],
                    whl[:, kk, 0:1, c0:c1].to_broadcast([P, 2, n]),
                    start=(kk == 0), stop=False, perf_mode=DR,
                )
            nc.tensor.matmul(
                out, xhl[:, 2 * i : 2 * i + 2, 0, l0 : l0 + P],
                whl[:, 2 * i : 2 * i + 2, 1, c0:c1],
                start=False, stop=(bias is None and i == KT // 2 - 1),
                perf_mode=DR,
            )
    if bias is not None:
        ones_row, brow = bias
        nc.tensor.matmul(out, ones_row, brow, start=False, stop=True)


def _body(ctx, tc, x_d, qkvw_d, qkvb_d, projw_d, projb_d, pm_d, y_d,
          L, LCH, NCH, NSUB, has_qkv_b, has_proj_b):
    nc = tc.nc
    iqkv = 1.0 / (SX * SW)
    iy = 1.0 / (SA * SP)

    persist = ctx.enter_context(tc.tile_pool(name="persist", bufs=1))

    ident16 = persist.tile([P, P], F16, tag="ident16", name="ident16")[:]
    nc.gpsimd.memset(ident16, 0.0)
    nc.gpsimd.affine_select(
        out=ident16, in_=ident16, compare_op=AL.not_equal, fill=1.0,
        base=0, pattern=[[-1, P]], channel_multiplier=1,
    )
    ones1 = persist.tile([1, P], F16, tag="ones1", name="ones1")[:]
    nc.gpsimd.memset(ones1, 1.0)
    epscol = persist.tile([P, 1], F16, tag="epscol", name="epscol")[:]
    nc.gpsimd.memset(epscol, EPS)
    epsb = persist.tile([P, 1], F32, tag="epsb", name="epsb")[:]
    nc.gpsimd.memset(epsb, SKP * EPS)

    whl_qk = persist.tile([P, KT, 2, 2 * DIM], F8, tag="whl_qk", name="whl_qk")[:]
    wvhl = persist.tile([P, KT, 2, DIM], F8, tag="wvhl", name="wvhl")[:]
    wphl = persist.tile([P, KT, 2, DIM], F8, tag="wphl", name="wphl")[:]
    # folded blockdiag pm for the fp8 DoubleRow k-feature GEMM:
    # slice 0 = [RATIO*pmT | 0] (c rows 0:64), slice 1 = [0 | RATIO*pmT]
    pmbd = persist.tile([P, 2, 2 * M], F8, tag="pmbd", name="pmbd")[:]
    pmt2 = persist.tile([P, M], F16, tag="pmt2", name="pmt2")[:]
    qt_sb = persist.tile([P, NPAIR, L], F16, tag="qt_sb", name="qt_sb")[:]
    kvm = persist.tile([P, 4, 6, D + 1], F16, tag="kvm", name="kvm")[:]
    kvmcs = persist.tile([1, 4, 6, D + 1], F16, tag="kvmcs", name="kvmcs")[:]

    if has_qkv_b:
        qkb = persist.tile([P, 2 * KT], F32, tag="qkb", name="qkb")[:]
        nc.sync.dma_start(qkb, qkvb_d.rearrange("(t p) -> p t", p=P)[:, 0 : 2 * KT])
        qkbk = persist.tile([P, KT], F32, tag="qkbk", name="qkbk")[:]
        nc.scalar.activation(qkbk, qkb[:, KT : 2 * KT], AF.Copy, scale=SKT)
        vbf = persist.tile([1, DIM], F32, tag="vbf", name="vbf")[:]
        nc.sync.dma_start(vbf, qkvb_d[2 * DIM : 3 * DIM].unsqueeze(0))
        vb_row = persist.tile([1, DIM], F16, tag="vb_row", name="vb_row")[:]
        nc.scalar.activation(vb_row, vbf, AF.Copy, scale=SX * SW)
    if has_proj_b:
        pbf = persist.tile([1, DIM], F32, tag="pbf", name="pbf")[:]
        nc.sync.dma_start(pbf, projb_d.unsqueeze(0))
        pb_row = persist.tile([1, DIM], F16, tag="pb_row", name="pb_row")[:]
        nc.scalar.activation(pb_row, pbf, AF.Copy, scale=SA * SP)

    vsb = persist.tile([P, 2, NSUB, H, D + 1], F16, tag="vsb", name="vsb")[:]
    nc.gpsimd.memset(vsb[:, :, :, :, D : D + 1], 1.0)

    # kv accumulator in SBUF fp32 (indexed by ai = h*2+mt)
    kv_acc = persist.tile([P, 2 * H, D + 1], F32, tag="kv_acc", name="kv_acc")[:]

    # pass-2 q-feature tiles, double-buffered by chunk parity
    qp2 = [persist.tile([P, H, 2, LCH], F16, tag=f"qp2_{i}", name=f"qp2_{i}")[:]
           for i in range(2)]

    def emit_qp(ich, heads, psum_pool, tag):
        l0 = ich * LCH
        qp_sb = qp2[ich % 2]
        for h in heads:
            p, h2 = h // 2, h % 2
            for mt in range(2):
                qps = psum_pool.tile([P, LCH], F32, tag=tag, name="qps")[:]
                nc.tensor.matmul(
                    qps,
                    pmt2[h2 * D : (h2 + 1) * D, mt * P : (mt + 1) * P],
                    qt_sb[h2 * D : (h2 + 1) * D, p, l0 : l0 + LCH],
                    start=True, stop=True)
                eng = _QP_ENG[h * 2 + mt]
                if eng == "act":
                    # plain relu; eps restored via rank-1 kvmcs in num
                    nc.scalar.activation(qp_sb[:, h, mt, :], qps, AF.Relu)
                else:
                    nc.vector.tensor_scalar(qp_sb[:, h, mt, :], qps,
                                            EPS, EPS, AL.add, AL.max)

    if True:
        with tc.tile_pool(name="p1x", bufs=2) as xp, \
             tc.tile_pool(name="p1xhl", bufs=2) as xhlp, \
             tc.tile_pool(name="p1kt", bufs=3) as ktp, \
             tc.tile_pool(name="p1kp", bufs=8) as kpp, \
             tc.tile_pool(name="wprep", bufs=3) as wpool, \
             tc.tile_pool(name="p1work", bufs=8, space="PSUM") as work:

            xnats = {}
            xhls = {}

            def dma_x(ich):
                l0 = ich * LCH
                xnat = xp.tile([P, NSUB, DIM], F16, tag="xnat", name="xnat")[:]
                nc.gpsimd.dma_start(
                    xnat,
                    x_d[l0 : l0 + LCH, :].rearrange("(s p) k -> p s k", p=P))
                xnats[ich] = xnat

            xhl_cur = {}

            def transp_x(ich, kks=range(KT)):
                if ich in xhl_cur:
                    xnat, xhl = xhl_cur[ich]
                else:
                    xnat = xnats.pop(ich)
                    xhl = xhlp.tile([P, KT, 2, LCH], F8, tag="xhl",
                                    name="xhl")[:]
                    xhl_cur[ich] = (xnat, xhl)
                for kk in kks:
                    tp = work.tile([P, 512], F16, tag="work", name="ttp")[:]
                    for s in range(NSUB):
                        nc.tensor.transpose(
                            tp[:, s * P : (s + 1) * P],
                            xnat[:, s, kk * P : (kk + 1) * P], ident16)
                    nc.scalar.activation(xhl[:, kk, 0, :], tp, AF.Copy,
                                         scale=SX)
                    nc.vector.scalar_tensor_tensor(
                        xhl[:, kk, 1, :], tp, SX, xhl[:, kk, 0, :],
                        AL.mult, AL.subtract)
                xhls[ich] = xhl

            def emit_v(ich, subs):
                vs = ich % 2
                xhl = xhls[ich]
                for s in subs:
                    for ci in range(2):
                        c0 = ci * 384
                        vps = work.tile([P, 512], F32, tag="work", name="vps")[:]
                        bias = None
                        if has_qkv_b:
                            bias = (ones1, vb_row[:, c0 : c0 + 384])
                        _dr_gemm(nc, vps[:, 0:384], wvhl, xhl, (c0, c0 + 384),
                                 lt=s, bias=bias)
                        nc.scalar.activation(
                            vsb[:, vs, s, 6 * ci : 6 * ci + 6, 0:D],
                            vps[:, 0:384].rearrange("p (h d) -> p h d", d=D),
                            AF.Copy, scale=iqkv)

            def emit_kT(ich, p):
                ktps = work.tile([P, 512], F32, tag="work", name="ktps")[:]
                _dr_gemm(nc, ktps, whl_qk, xhls[ich],
                         (DIM + p * P, DIM + (p + 1) * P))
                # fold [128,512] -> [64, 2, 512] fp8 (scaled) for DoubleRow
                kt = ktp.tile([P, 2, LCH], F8, tag="kt", name="kt")[:]
                for h2 in range(2):
                    if has_qkv_b:
                        nc.scalar.activation(
                            kt[0:D, h2, :], ktps[h2 * D : (h2 + 1) * D, :],
                            AF.Identity,
                            bias=qkbk[h2 * D : (h2 + 1) * D, p : p + 1],
                            scale=SKT * iqkv)
                    else:
                        nc.scalar.activation(
                            kt[0:D, h2, :], ktps[h2 * D : (h2 + 1) * D, :],
                            AF.Copy, scale=SKT * iqkv)
                return kt

            def emit_kp(p, kt):
                kps = []
                for lt in range(NSUB):
                    kpps = work.tile([P, 512], F32, tag="work", name="kpps")[:]
                    nc.tensor.matmul(kpps,
                                     kt[0:D, :, lt * P : (lt + 1) * P],
                                     pmbd[0:D], start=True, stop=True,
                                     perf_mode=DR)
                    kp = kpp.tile([P, 2 * M], F16, tag="kp", name="kp")[:]
                    # k-feature path is scaled by SKP; it cancels in num/den.
                    # ACT-assigned tiles use relu(z+eps) ~ relu(z)+eps
                    # (differs by <= eps only for z in (-eps, 0)); exact
                    # max(z+eps, eps) on DVE for the rest
                    if p == 3:
                        nc.scalar.activation(kp, kpps, AF.Relu, bias=epsb)
                    else:
                        nc.vector.tensor_scalar(kp, kpps, SKP * EPS, SKP * EPS,
                                                AL.add, AL.max)
                    kps.append(kp)
                return kps

            def emit_qT(ich, p):
                l0 = ich * LCH
                qtps = work.tile([P, 512], F32, tag="work", name="qtps")[:]
                _dr_gemm(nc, qtps, whl_qk, xhls[ich], (p * P, (p + 1) * P))
                if has_qkv_b:
                    nc.scalar.activation(qt_sb[:, p, l0 : l0 + LCH], qtps,
                                         AF.Identity,
                                         bias=qkb[:, p : p + 1], scale=iqkv)
                else:
                    nc.scalar.activation(qt_sb[:, p, l0 : l0 + LCH], qtps,
                                         AF.Copy, scale=iqkv)

            def emit_kv(ich, p, kps):
                vs = ich % 2
                kvp = work.tile([P, 4, D + 1], F32, tag="work", name="kvp")[:]
                for lt in range(NSUB):
                    kp = kps[lt]
                    for h2 in range(2):
                        h = 2 * p + h2
                        for mt in range(2):
                            j = h2 * 2 + mt
                            nc.tensor.matmul(
                                kvp[:, j, :],
                                kp[:, j * P : (j + 1) * P],
                                vsb[:, vs, lt, h, :],
                                start=(lt == 0 and j == 0),
                                stop=(lt == NSUB - 1 and j == 3),
                            )
                nc.vector.tensor_tensor(
                    kv_acc[:, 4 * p : 4 * p + 4, :], kvp,
                    kv_acc[:, 4 * p : 4 * p + 4, :], AL.add)

            # ---- prep: weight DMA blocks interleaved with chunk-0/1 work ----
            def prep_w_blocks(src, nrows, dst, dst_off, scale):
                blocks = []
                nt = nrows // P
                c0 = 0
                while c0 < nt:
                    bs = min(3, nt - c0)
                    st = {}

                    def bdma(c0=c0, bs=bs, st=st):
                        wnat = wpool.tile([P, 3, DIM], F16, tag="wnat",
                                          name="wnat")[:]
                        nc.gpsimd.dma_start(
                            wnat[:, 0:bs, :],
                            src[c0 * P : (c0 + bs) * P, :].rearrange(
                                "(s p) k -> p s k", p=P))
                        st["wnat"] = wnat

                    def bcomp(c0=c0, bs=bs, st=st):
                        wnat = st["wnat"]
                        for kk in range(KT):
                            tp = work.tile([P, 512], F16, tag="work",
                                           name="ptp")[:]
                            for j in range(bs):
                                nc.tensor.transpose(
                                    tp[:, j * P : (j + 1) * P],
                                    wnat[:, j, kk * P : (kk + 1) * P], ident16)
                            hi = dst[:, kk, 0,
                                     dst_off + c0 * P : dst_off + (c0 + bs) * P]
                            nc.scalar.activation(hi, tp[:, 0 : bs * P], AF.Copy,
                                                 scale=scale)
                            nc.vector.scalar_tensor_tensor(
                                dst[:, kk, 1,
                                    dst_off + c0 * P : dst_off + (c0 + bs) * P],
                                tp[:, 0 : bs * P], scale, hi,
                                AL.mult, AL.subtract)

                    blocks.append((bdma, bcomp))
                    c0 += bs
                return blocks

            pm_st = {}

            def prep_pm_dma():
                pmn = wpool.tile([P, 2, D], F16, tag="pmn", name="pmn")[:]
                nc.gpsimd.dma_start(pmn, pm_d.rearrange("(s p) d -> p s d", p=P))
                pm_st["pmn"] = pmn

            def prep_pm():
                pmn = pm_st["pmn"]
                tp = work.tile([P, 512], F16, tag="work", name="ptp")[:]
                for s in range(2):
                    nc.tensor.transpose(tp[0:D, s * P : (s + 1) * P],
                                        pmn[:, s, :], ident16)
                nc.gpsimd.memset(pmbd, 0.0)
                nc.scalar.activation(pmbd[0:D, 0, 0:M], tp[0:D, 0:M], AF.Copy,
                                     scale=SPM * RATIO)
                nc.scalar.activation(pmbd[0:D, 1, M : 2 * M], tp[0:D, 0:M],
                                     AF.Copy, scale=SPM * RATIO)
                nc.scalar.activation(pmt2[0:D, :], tp[0:D, 0:M], AF.Copy,
                                     scale=RATIO)
                nc.scalar.activation(pmt2[D:P, :], tp[0:D, 0:M], AF.Copy,
                                     scale=RATIO)

            dma_x(0)
            wv = prep_w_blocks(qkvw_d[2 * DIM : 3 * DIM, :], DIM, wvhl, 0, SW)
            wqk_k = prep_w_blocks(qkvw_d[DIM : 2 * DIM, :], DIM, whl_qk,
                                  DIM, SW)
            wqk_q = prep_w_blocks(qkvw_d[0:DIM, :], DIM, whl_qk, 0, SW)
            wp = prep_w_blocks(projw_d, DIM, wphl, 0, SP)

            nc.gpsimd.memset(kv_acc, 0.0)
            blocks = wv + wqk_k + wqk_q + wp
            bst = {"dma": 0, "comp": 0}

            def bdma_next():
                if bst["dma"] < len(blocks):
                    blocks[bst["dma"]][0]()
                    bst["dma"] += 1

            def bcomp_next():
                if bst["comp"] < len(blocks):
                    blocks[bst["comp"]][1]()
                    bst["comp"] += 1
                    bdma_next()

            nop = lambda: None
            # phase A: wv + k-part of wqk; q-part and proj stream into chunk 0
            nA = len(wv) + len(wqk_k)
            fillers = {
                0: [lambda: transp_x(0, range(0, 3)),
                    (lambda: dma_x(1)) if 1 < NCH else nop],
                1: [lambda: transp_x(0, range(3, KT)), prep_pm],
                2: [lambda: emit_v(0, (0,)), lambda: emit_v(0, (1,)),
                    (lambda: transp_x(1, range(0, 3))) if 1 < NCH else nop],
                3: [lambda: emit_v(0, (2,)), lambda: emit_v(0, (3,)),
                    (lambda: transp_x(1, range(3, KT))) if 1 < NCH else nop,
                    (lambda: dma_x(2)) if 2 < NCH else nop],
            }
            bdma_next()
            prep_pm_dma()
            bdma_next()
            # PE p-state warmup: burn the cold DMA-wait ramping the clock
            wu = work.tile([P, 512], F16, tag="work", name="wu")[:]
            for _ in range(12):
                for s in range(4):
                    nc.tensor.transpose(wu[:, s * P : (s + 1) * P], ident16,
                                        ident16)
            for i in range(nA):
                for f in fillers.get(i, []):
                    f()
                bcomp_next()

            # ---- pass 1 main loop ----
            for ich in range(NCH):
                first = ich == 0
                pend_kt = None
                pend = None
                for p in range(NPAIR):
                    kt = emit_kT(ich, p)
                    if first:
                        # stream remaining weight blocks (q-part + proj)
                        bcomp_next()
                        if p >= 3:
                            emit_qT(0, p - 3)
                    else:
                        emit_qT(ich, p)
                    if pend is not None:
                        emit_kv(ich, pend[0], pend[1])
                        if ich == NCH - 1:
                            pp = pend[0]
                            nc.scalar.activation(
                                kvm.rearrange("p b j c -> p (b j) c")[
                                    :, 4 * pp : 4 * pp + 4, :],
                                kv_acc[:, 4 * pp : 4 * pp + 4, :],
                                AF.Copy, scale=1.0 / SKP)
                        pend = None
                    if pend_kt is not None:
                        pend = (pend_kt[0], emit_kp(pend_kt[0], pend_kt[1]))
                    pend_kt = (p, kt)
                    if ich == NCH - 1 and not first:
                        # chunk-0 q features computed here so pass 2 starts hot
                        emit_qp(0, (2 * p, 2 * p + 1), work, "work")
                    if p == 0 and 1 <= ich and ich + 1 < NCH:
                        transp_x(ich + 1, range(0, 3))
                    if p == 2 and 1 <= ich and ich + 1 < NCH:
                        transp_x(ich + 1, range(3, KT))
                    if p == 3 and 1 <= ich and ich + 2 < NCH:
                        dma_x(ich + 2)
                    if p >= 3 and ich + 1 < NCH:
                        emit_v(ich + 1, (p - 3,))
                if pend is not None:
                    emit_kv(ich, pend[0], pend[1])
                    if ich == NCH - 1:
                        pp = pend[0]
                        nc.scalar.activation(
                            kvm.rearrange("p b j c -> p (b j) c")[
                                :, 4 * pp : 4 * pp + 4, :],
                            kv_acc[:, 4 * pp : 4 * pp + 4, :],
                            AF.Copy, scale=1.0 / SKP)
                pend = (pend_kt[0], emit_kp(pend_kt[0], pend_kt[1]))
                if ich + 1 < NCH:
                    emit_v(ich + 1, (3,))
                emit_kv(ich, pend[0], pend[1])
                if ich == NCH - 1:
                    nc.scalar.activation(
                        kvm.rearrange("p b j c -> p (b j) c")[:, 20:24, :],
                        kv_acc[:, 20:24, :], AF.Copy, scale=1.0 / SKP)
                if first:
                    for p3 in range(3, NPAIR):
                        emit_qT(0, p3)
                    if NCH == 1:
                        emit_qp(0, range(H), work, "work")
                xhls.pop(ich)

    with tc.tile_pool(name="csps", bufs=1, space="PSUM") as cspool:
        css = []
        for b in range(4):
            cs = cspool.tile([1, 6, D + 1], F32, tag=f"cs{b}", name="cs")[:]
            for j in range(6):
                nc.tensor.matmul(cs[:, j, :], epscol, kvm[:, b, j, :],
                                 start=(j == 0), stop=(j == 5))
            css.append(cs)
        for b in range(4):
            nc.scalar.copy(kvmcs[:, b], css[b])

    # ---- pass 2 ----
    with tc.tile_pool(name="p2attn", bufs=2) as atp_sb, \
         tc.tile_pool(name="p2rd", bufs=2) as rdp, \
         tc.tile_pool(name="p2ahl", bufs=2) as ahlp, \
         tc.tile_pool(name="p2y", bufs=2) as yp, \
         tc.tile_pool(name="ps2qp", bufs=3, space="PSUM") as qppsum, \
         tc.tile_pool(name="ps2nm", bufs=2, space="PSUM") as numpsum, \
         tc.tile_pool(name="ps2at", bufs=1, space="PSUM") as atpsum, \
         tc.tile_pool(name="ps2y", bufs=1, space="PSUM") as ypsum:

        def emit_y(ich, ahl, lt):
            l0 = ich * LCH
            yps = ypsum.tile([P, DIM], F32, tag="yps", name="yps")[:]
            for c0, c1 in ((0, 512), (512, DIM)):
                b = (ones1, pb_row[:, c0:c1]) if has_proj_b else None
                _dr_gemm(nc, yps[:, c0:c1], wphl, ahl, (c0, c1), lt=lt, bias=b)
            ysb = yp.tile([P, DIM], F32, tag="ysb", name="ysb")[:]
            nc.scalar.activation(ysb, yps, AF.Copy, scale=iy)
            nc.sync.dma_start(y_d[l0 + lt * P : l0 + (lt + 1) * P, :], ysb)

        pend_y = []
        for ich in range(NCH):
            qp_sb = qp2[ich % 2]
            ahl = ahlp.tile([P, KT, 2, LCH], F8, tag="ahl", name="ahl")[:]
            for lt in range(NSUB):
                nmps = [numpsum.tile([P, 6, D + 1], F32, tag="nm", name="nmps")[:]
                        for _ in range(2)]
                for h in range(H):
                    g = h // 6
                    for mt in range(2):
                        ai = h * 2 + mt
                        if _QP_ENG[ai] == "act" and h % 6 != 0:
                            nc.tensor.matmul(
                                nmps[g][:, h % 6, :], ones1,
                                kvmcs[0:1, ai // 6, ai % 6, :],
                                start=False, stop=False)
                    for mt in range(2):
                        ai = h * 2 + mt
                        nc.tensor.matmul(
                            nmps[g][:, h % 6, :],
                            qp_sb[:, h, mt, lt * P : (lt + 1) * P],
                            kvm[:, ai // 6, ai % 6, :],
                            start=(mt == 0 and h % 6 == 0),
                            stop=(mt == 1 and h % 6 == 5))
                rd = rdp.tile([P, H], F32, tag="rd", name="rd")[:]
                attn = atp_sb.tile([P, H, D], F16, tag="attn", name="attn")[:]
                for g in range(2):
                    nc.vector.reciprocal(rd[:, g * 6 : (g + 1) * 6],
                                         nmps[g][:, :, D])
                    nc.vector.tensor_tensor(
                        attn[:, g * 6 : (g + 1) * 6, :],
                        nmps[g][:, :, 0:D],
                        rd[:, g * 6 : (g + 1) * 6, None].to_broadcast([P, 6, D]),
                        AL.mult)
                if ich + 1 < NCH:
                    emit_qp(ich + 1, range(3 * lt, 3 * lt + 3), qppsum, "qps")
                if len(pend_y) >= (2 if ich + 1 < NCH else 1):
                    emit_y(*pend_y.pop(0))
                atps = atpsum.tile([P, DIM], F16, tag="at", name="atps")[:]
                for kk in range(KT):
                    nc.tensor.transpose(atps[:, kk * P : (kk + 1) * P],
                                        attn[:, 2 * kk : 2 * kk + 2, :], ident16)
                a3 = atps.rearrange("p (k l) -> p k l", k=KT)
                nc.scalar.activation(ahl[:, :, 0, lt * P : (lt + 1) * P], a3,
                                     AF.Copy, scale=SA)
                nc.vector.scalar_tensor_tensor(
                    ahl[:, :, 1, lt * P : (lt + 1) * P], a3, SA,
                    ahl[:, :, 0, lt * P : (lt + 1) * P], AL.mult, AL.subtract)
                pend_y.append((ich, ahl, lt))
        for args in pend_y:
            emit_y(*args)


_CACHE = {}


def _get_nc(L=4096, hqb=False, hpb=False):
    key = ("nc", L, hqb, hpb)
    if key not in _CACHE:
        _CACHE[key] = build(L, hqb, hpb)
    return _CACHE[key]


last_exec_time_ns = None
last_profile = None


def kernel(x, qkv_w, qkv_b, proj_w, proj_b, proj_mat):
    global last_exec_time_ns, last_profile
    from concourse.bass_utils import run_bass_kernel_spmd

    x = np.asarray(x, np.float32)
    B, L, _ = x.shape
    hqb = bool(np.any(np.asarray(qkv_b)))
    hpb = bool(np.any(np.asarray(proj_b)))
    nc = _get_nc(L, hqb, hpb)
    base = {
        "qkv_w": np.ascontiguousarray(np.asarray(qkv_w, np.float32)),
        "qkv_b": np.ascontiguousarray(np.asarray(qkv_b, np.float32)),
        "proj_w": np.ascontiguousarray(np.asarray(proj_w, np.float32)),
        "proj_b": np.ascontiguousarray(np.asarray(proj_b, np.float32)),
        "proj_mat": np.ascontiguousarray(np.asarray(proj_mat, np.float32)),
    }
    in_maps = [dict(base, x=np.ascontiguousarray(x[b])) for b in range(B)]
    trace = bool(int(os.environ.get("KERNEL_TRACE", "0")))
    res = run_bass_kernel_spmd(nc, in_maps, core_ids=list(range(B)), trace=trace)
    last_exec_time_ns = res.exec_time_ns
    last_profile = res.profile_json
    return np.stack([res.results[b]["y"] for b in range(B)], axis=0)


def _ref_np(x, qkv_w, qkv_b, proj_w, proj_b, proj_mat):
    Ls = x.shape[0]
    qkv = x @ qkv_w.T + qkv_b
    qkv = qkv.reshape(Ls, 3, H, D)
    q, k, v = qkv[:, 0], qkv[:, 1], qkv[:, 2]
    qp = np.maximum(RATIO * np.einsum("lhd,md->lhm", q, proj_mat), 0) + EPS
    kp = np.maximum(RATIO * np.einsum("lhd,md->lhm", k, proj_mat), 0) + EPS
    kv = np.einsum("lhm,lhd->hmd", kp, v)
    ks = kp.sum(axis=0)
    num = np.einsum("lhm,hmd->lhd", qp, kv)
    den = np.einsum("lhm,hm->lh", qp, ks)
    out = (num / den[..., None]).reshape(Ls, DIM)
    return out @ proj_w.T + proj_b


if __name__ == "__main__":
    from concourse.bass_interp import CoreSim

    Ls = int(os.environ.get("SIM_L", "512"))
    use_bias = bool(int(os.environ.get("SIM_BIAS", "1")))
    rng = np.random.default_rng(0)
    x = rng.standard_normal((Ls, DIM), dtype=np.float32)
    qkv_w = (rng.standard_normal((3 * DIM, DIM), dtype=np.float32) * DIM**-0.5)
    qkv_b = (rng.standard_normal(3 * DIM, dtype=np.float32) * 0.1
             if use_bias else np.zeros(3 * DIM, np.float32))
    proj_w = (rng.standard_normal((DIM, DIM), dtype=np.float32) * DIM**-0.5)
    proj_b = (rng.standard_normal(DIM, dtype=np.float32) * 0.1
              if use_bias else np.zeros(DIM, np.float32))
    proj_mat = rng.standard_normal((M, D), dtype=np.float32)

    print(f"building L={Ls} bias={use_bias} ...")
    nc = build(Ls, use_bias, use_bias)
    print("simulating ...")
    sim = CoreSim(nc)
    for name, arr in [("x", x), ("qkv_w", qkv_w), ("qkv_b", qkv_b),
                      ("proj_w", proj_w), ("proj_b", proj_b),
                      ("proj_mat", proj_mat)]:
        sim.tensor(name)[:] = arr
    sim.simulate(check_with_hw=False)
    got = np.array(sim.tensor("y"))
    want = _ref_np(x, qkv_w, qkv_b, proj_w, proj_b, proj_mat)
    err = np.abs(got - want)
    rel = np.linalg.norm(got - want) / np.linalg.norm(want)
    print("max abs err:", err.max(), " rel fro err:", rel)
    assert rel < 2e-2, "sim mismatch"
    print("SIM OK")


# revision 16
# speedup vs baseline: 1.0273x; 1.0020x over previous
"""FAVOR+ (Performer) non-causal linear attention on 8 Trainium2 NeuronCores.

Sharding: data-parallel over batch B=8 -> one batch element per core.

Per-core pipeline (L=4096, DIM=768, H=12, D=64, M=256):
  prep : cast-DMA weights to fp16, PE-transpose to feature-major, split into
         fp8e4m3 hi/lo pairs (scaled) for DoubleRow matmuls; DMA blocks
         interleaved with chunk-0/1 transposes and v so PE never starves
  pass1: per 512-row chunk: cast-DMA x to fp16; PE-transpose -> xT hi/lo fp8;
         kT/qT/v via fp8 DoubleRow hi/lo-compensated GEMMs (4.5 cyc per
         128x512 output tile instead of 6); k features fp16 with blockdiag pm
         (2 heads per matmul); kv accumulated m-major [m,65] into 4 persistent
         PSUM banks across all chunks (ones-augmented v gives k_sum for
         free); qT staged in SBUF fp16 (no DRAM round trip).  Emission is
         software-pipelined: transposes/v run 2 chunks ahead; kv trails one
         pair behind its kp conversion.
  mid  : kv PSUM -> fp16 SBUF (already m-major); eps*colsum(kv) rows for the
         ACT-assigned q-feature heads
  pass2: q features m-major fp16 (computed one chunk ahead, spread across the
         l-tile loop; relu+eps on DVE/Pool, plain relu on ACT with the eps
         restored by a rank-1 matmul into num); num L-major [l,65] (den =
         col 64); reciprocal + broadcast multiply on DVE; attn PE-transposed
         to feature-major, split fp8 hi/lo; y via DoubleRow GEMM -> DMA out
"""

import math
import os
import sys
from contextlib import ExitStack

import numpy as np

for _p in ("/opt/trn_rl_repo",):
    if _p not in sys.path and os.path.isdir(_p):
        sys.path.insert(0, _p)

import concourse.bass as bass  # noqa: E402
import concourse.mybir as mybir  # noqa: E402
import concourse.tile as tile  # noqa: E402
from concourse import bacc  # noqa: E402

P = 128
DIM = 768
H = 12
D = 64
M = 256
KT = DIM // P   # 6 contraction c-tiles
NPAIR = H // 2  # 6 head pairs
EPS = 1e-3
RATIO = 1.0 / math.sqrt(float(M))

SX = 16.0    # x ~ N(0,1)
SW = 32.0    # qkv_w ~ N(0, 1/768)
SA = 64.0    # attn ~ 0.1
SP = 32.0    # proj_w ~ N(0, 1/768)
SKT = 16.0   # kT ~ N(0,1) -> fp8 for the k-feature GEMM
SPM = 32.0   # RATIO*pm ~ N(0, 1/256) -> fp8
SKP = SKT * SPM  # k-feature path runs in this scaled domain until kvm

F32 = mybir.dt.float32
F16 = mybir.dt.float16
F8 = mybir.dt.float8e4
AL = mybir.AluOpType
AF = mybir.ActivationFunctionType
DR = mybir.MatmulPerfMode.DoubleRow

# pass-2 q-feature conversion engine per (head, mtile) slot ai=h*2+mt.
# Slots of the group-starting heads (ai 0,1,12,13) stay off ACT so each num
# PSUM group's first instruction is its start matmul.
_QP_ENG = {}
_c = 0
for _ai in range(2 * H):
    if _ai in (0, 1, 12, 13):
        _QP_ENG[_ai] = "dve"
    else:
        _QP_ENG[_ai] = ("dve", "act", "act")[_c % 3]
        _c += 1


def build(L=4096, has_qkv_b=False, has_proj_b=False):
    LCH = 512
    NCH = L // LCH
    NSUB = LCH // P  # 4

    nc = bacc.Bacc("TRN2", target_bir_lowering=False, debug=False)
    x_d = nc.dram_tensor("x", [L, DIM], F32, kind="ExternalInput").ap()
    qkvw_d = nc.dram_tensor("qkv_w", [3 * DIM, DIM], F32, kind="ExternalInput").ap()
    qkvb_d = nc.dram_tensor("qkv_b", [3 * DIM], F32, kind="ExternalInput").ap()
    projw_d = nc.dram_tensor("proj_w", [DIM, DIM], F32, kind="ExternalInput").ap()
    projb_d = nc.dram_tensor("proj_b", [DIM], F32, kind="ExternalInput").ap()
    pm_d = nc.dram_tensor("proj_mat", [M, D], F32, kind="ExternalInput").ap()
    y_d = nc.dram_tensor("y", [L, DIM], F32, kind="ExternalOutput").ap()

    with tile.TileContext(nc) as tc:
        with ExitStack() as ctx:
            _body(ctx, tc, x_d, qkvw_d, qkvb_d, projw_d, projb_d, pm_d, y_d,
                  L, LCH, NCH, NSUB, has_qkv_b, has_proj_b)
    nc.compile()
    return nc


def _dr_gemm(nc, out, whl, xhl, wcols, lt=None, bias=None):
    """Accumulating fp8 DoubleRow hi/lo-compensated GEMM over 768 contraction.

    whl/xhl: [128, KT, 2, *] fp8 with hi at [:,kk,0,:], lo at [:,kk,1,:].
    lt=None : out[wc, l]; stationary = whl cols wcols, moving = xhl  (kT/qT)
    lt given: out[l, wc]; stationary = xhl l-tile cols, moving = whl (v/y)
    """
    c0, c1 = wcols
    n = c1 - c0
    if lt is None:
        for i in range(KT // 2):
            for kk in (2 * i, 2 * i + 1):
                nc.tensor.matmul(
                    out, whl[:, kk, :, c0:c1],
                    xhl[:, kk, 0:1, :].to_broadcast([P, 2, out.shape[-1]]),
                    start=(kk == 0), stop=False, perf_mode=DR,
                )
            nc.tensor.matmul(
                out, whl[:, 2 * i : 2 * i + 2, 0, c0:c1],
                xhl[:, 2 * i : 2 * i + 2, 1, :],
                start=False, stop=(bias is None and i == KT // 2 - 1),
                perf_mode=DR,
            )
    else:
        l0 = lt * P
        for i in range(KT // 2):
            for kk in (2 * i, 2 * i + 1):
                nc.tensor.matmul(
                    out, xhl[:, kk, :, l0 : l0 + P],
                    whl[:, kk, 0:1, c0:c1].to_broadcast([P, 2, n]),
                    start=(kk == 0), stop=False, perf_mode=DR,
                )
            nc.tensor.matmul(
                out, xhl[:, 2 * i : 2 * i + 2, 0, l0 : l0 + P],
                whl[:, 2 * i : 2 * i + 2, 1, c0:c1],
                start=False, stop=(bias is None and i == KT // 2 - 1),
                perf_mode=DR,
            )
    if bias is not None:
        ones_row, brow = bias
        nc.tensor.matmul(out, ones_row, brow, start=False, stop=True)


def _body(ctx, tc, x_d, qkvw_d, qkvb_d, projw_d, projb_d, pm_d, y_d,
          L, LCH, NCH, NSUB, has_qkv_b, has_proj_b):
    nc = tc.nc
    iqkv = 1.0 / (SX * SW)
    iy = 1.0 / (SA * SP)

    persist = ctx.enter_context(tc.tile_pool(name="persist", bufs=1))

    ident16 = persist.tile([P, P], F16, tag="ident16", name="ident16")[:]
    nc.gpsimd.memset(ident16, 0.0)
    nc.gpsimd.affine_select(
        out=ident16, in_=ident16, compare_op=AL.not_equal, fill=1.0,
        base=0, pattern=[[-1, P]], channel_multiplier=1,
    )
    ones1 = persist.tile([1, P], F16, tag="ones1", name="ones1")[:]
    nc.gpsimd.memset(ones1, 1.0)
    epscol = persist.tile([P, 1], F16, tag="epscol", name="epscol")[:]
    nc.gpsimd.memset(epscol, EPS)
    epsb = persist.tile([P, 1], F32, tag="epsb", name="epsb")[:]
    nc.gpsimd.memset(epsb, SKP * EPS)

    whl_qk = persist.tile([P, KT, 2, 2 * DIM], F8, tag="whl_qk", name="whl_qk")[:]
    wvhl = persist.tile([P, KT, 2, DIM], F8, tag="wvhl", name="wvhl")[:]
    wphl = persist.tile([P, KT, 2, DIM], F8, tag="wphl", name="wphl")[:]
    # folded blockdiag pm for the fp8 DoubleRow k-feature GEMM:
    # slice 0 = [RATIO*pmT | 0] (c rows 0:64), slice 1 = [0 | RATIO*pmT]
    pmbd = persist.tile([P, 2, 2 * M], F8, tag="pmbd", name="pmbd")[:]
    pmt2 = persist.tile([P, M], F16, tag="pmt2", name="pmt2")[:]
    qt_sb = persist.tile([P, NPAIR, L], F16, tag="qt_sb", name="qt_sb")[:]
    kvm = persist.tile([P, 4, 6, D + 1], F16, tag="kvm", name="kvm")[:]
    kvmcs = persist.tile([1, 4, 6, D + 1], F16, tag="kvmcs", name="kvmcs")[:]

    if has_qkv_b:
        qkb = persist.tile([P, 2 * KT], F32, tag="qkb", name="qkb")[:]
        nc.sync.dma_start(qkb, qkvb_d.rearrange("(t p) -> p t", p=P)[:, 0 : 2 * KT])
        qkbk = persist.tile([P, KT], F32, tag="qkbk", name="qkbk")[:]
        nc.scalar.activation(qkbk, qkb[:, KT : 2 * KT], AF.Copy, scale=SKT)
        vbf = persist.tile([1, DIM], F32, tag="vbf", name="vbf")[:]
        nc.sync.dma_start(vbf, qkvb_d[2 * DIM : 3 * DIM].unsqueeze(0))
        vb_row = persist.tile([1, DIM], F16, tag="vb_row", name="vb_row")[:]
        nc.scalar.activation(vb_row, vbf, AF.Copy, scale=SX * SW)
    if has_proj_b:
        pbf = persist.tile([1, DIM], F32, tag="pbf", name="pbf")[:]
        nc.sync.dma_start(pbf, projb_d.unsqueeze(0))
        pb_row = persist.tile([1, DIM], F16, tag="pb_row", name="pb_row")[:]
        nc.scalar.activation(pb_row, pbf, AF.Copy, scale=SA * SP)

    vsb = persist.tile([P, 2, NSUB, H, D + 1], F16, tag="vsb", name="vsb")[:]
    nc.gpsimd.memset(vsb[:, :, :, :, D : D + 1], 1.0)

    # kv accumulator in SBUF fp32 (indexed by ai = h*2+mt)
    kv_acc = persist.tile([P, 2 * H, D + 1], F32, tag="kv_acc", name="kv_acc")[:]

    # pass-2 q-feature tiles, double-buffered by chunk parity
    qp2 = [persist.tile([P, H, 2, LCH], F16, tag=f"qp2_{i}", name=f"qp2_{i}")[:]
           for i in range(2)]

    def emit_qp(ich, heads, psum_pool, tag):
        l0 = ich * LCH
        qp_sb = qp2[ich % 2]
        for h in heads:
            p, h2 = h // 2, h % 2
            for mt in range(2):
                qps = psum_pool.tile([P, LCH], F32, tag=tag, name="qps")[:]
                nc.tensor.matmul(
                    qps,
                    pmt2[h2 * D : (h2 + 1) * D, mt * P : (mt + 1) * P],
                    qt_sb[h2 * D : (h2 + 1) * D, p, l0 : l0 + LCH],
                    start=True, stop=True)
                eng = _QP_ENG[h * 2 + mt]
                if eng == "act":
                    # plain relu; eps restored via rank-1 kvmcs in num
                    nc.scalar.activation(qp_sb[:, h, mt, :], qps, AF.Relu)
                else:
                    nc.vector.tensor_scalar(qp_sb[:, h, mt, :], qps,
                                            EPS, EPS, AL.add, AL.max)

    if True:
        with tc.tile_pool(name="p1x", bufs=2) as xp, \
             tc.tile_pool(name="p1xhl", bufs=2) as xhlp, \
             tc.tile_pool(name="p1kt", bufs=3) as ktp, \
             tc.tile_pool(name="p1kp", bufs=8) as kpp, \
             tc.tile_pool(name="wprep", bufs=3) as wpool, \
             tc.tile_pool(name="p1work", bufs=8, space="PSUM") as work:

            xnats = {}
            xhls = {}

            def dma_x(ich):
                l0 = ich * LCH
                xnat = xp.tile([P, NSUB, DIM], F16, tag="xnat", name="xnat")[:]
                nc.gpsimd.dma_start(
                    xnat,
                    x_d[l0 : l0 + LCH, :].rearrange("(s p) k -> p s k", p=P))
                xnats[ich] = xnat

            xhl_cur = {}

            def transp_x(ich, kks=range(KT)):
                if ich in xhl_cur:
                    xnat, xhl = xhl_cur[ich]
                else:
                    xnat = xnats.pop(ich)
                    xhl = xhlp.tile([P, KT, 2, LCH], F8, tag="xhl",
                                    name="xhl")[:]
                    xhl_cur[ich] = (xnat, xhl)
                for kk in kks:
                    tp = work.tile([P, 512], F16, tag="work", name="ttp")[:]
                    for s in range(NSUB):
                        nc.tensor.transpose(
                            tp[:, s * P : (s + 1) * P],
                            xnat[:, s, kk * P : (kk + 1) * P], ident16)
                    nc.scalar.activation(xhl[:, kk, 0, :], tp, AF.Copy,
                                         scale=SX)
                    nc.vector.scalar_tensor_tensor(
                        xhl[:, kk, 1, :], tp, SX, xhl[:, kk, 0, :],
                        AL.mult, AL.subtract)
                xhls[ich] = xhl

            def emit_v(ich, subs):
                vs = ich % 2
                xhl = xhls[ich]
                for s in subs:
                    for ci in range(2):
                        c0 = ci * 384
                        vps = work.tile([P, 512], F32, tag="work", name="vps")[:]
                        bias = None
                        if has_qkv_b:
                            bias = (ones1, vb_row[:, c0 : c0 + 384])
                        _dr_gemm(nc, vps[:, 0:384], wvhl, xhl, (c0, c0 + 384),
                                 lt=s, bias=bias)
                        nc.scalar.activation(
                            vsb[:, vs, s, 6 * ci : 6 * ci + 6, 0:D],
                            vps[:, 0:384].rearrange("p (h d) -> p h d", d=D),
                            AF.Copy, scale=iqkv)

            def emit_kT(ich, p):
                ktps = work.tile([P, 512], F32, tag="work", name="ktps")[:]
                _dr_gemm(nc, ktps, whl_qk, xhls[ich],
                         (DIM + p * P, DIM + (p + 1) * P))
                # fold [128,512] -> [64, 2, 512] fp8 (scaled) for DoubleRow
                kt = ktp.tile([P, 2, LCH], F8, tag="kt", name="kt")[:]
                for h2 in range(2):
                    if has_qkv_b:
                        nc.scalar.activation(
                            kt[0:D, h2, :], ktps[h2 * D : (h2 + 1) * D, :],
                            AF.Identity,
                            bias=qkbk[h2 * D : (h2 + 1) * D, p : p + 1],
                            scale=SKT * iqkv)
                    else:
                        nc.scalar.activation(
                            kt[0:D, h2, :], ktps[h2 * D : (h2 + 1) * D, :],
                            AF.Copy, scale=SKT * iqkv)
                return kt

            def emit_kp(p, kt):
                kps = []
                for lt in range(NSUB):
                    kpps = work.tile([P, 512], F32, tag="work", name="kpps")[:]
                    nc.tensor.matmul(kpps,
                                     kt[0:D, :, lt * P : (lt + 1) * P],
                                     pmbd[0:D], start=True, stop=True,
                                     perf_mode=DR)
                    kp = kpp.tile([P, 2 * M], F16, tag="kp", name="kp")[:]
                    # k-feature path is scaled by SKP; it cancels in num/den.
                    # ACT-assigned tiles use relu(z+eps) ~ relu(z)+eps
                    # (differs by <= eps only for z in (-eps, 0)); exact
                    # max(z+eps, eps) on DVE for the rest
                    if lt == 2:
                        nc.scalar.activation(kp, kpps, AF.Relu, bias=epsb)
                    else:
                        nc.vector.tensor_scalar(kp, kpps, SKP * EPS, SKP * EPS,
                                                AL.add, AL.max)
                    kps.append(kp)
                return kps

            def emit_qT(ich, p):
                l0 = ich * LCH
                qtps = work.tile([P, 512], F32, tag="work", name="qtps")[:]
                _dr_gemm(nc, qtps, whl_qk, xhls[ich], (p * P, (p + 1) * P))
                if has_qkv_b:
                    nc.scalar.activation(qt_sb[:, p, l0 : l0 + LCH], qtps,
                                         AF.Identity,
                                         bias=qkb[:, p : p + 1], scale=iqkv)
                else:
                    nc.scalar.activation(qt_sb[:, p, l0 : l0 + LCH], qtps,
                                         AF.Copy, scale=iqkv)

            def emit_kv(ich, p, kps):
                vs = ich % 2
                kvp = work.tile([P, 4, D + 1], F32, tag="work", name="kvp")[:]
                for lt in range(NSUB):
                    kp = kps[lt]
                    for h2 in range(2):
                        h = 2 * p + h2
                        for mt in range(2):
                            j = h2 * 2 + mt
                            nc.tensor.matmul(
                                kvp[:, j, :],
                                kp[:, j * P : (j + 1) * P],
                                vsb[:, vs, lt, h, :],
                                start=(lt == 0 and j == 0),
                                stop=(lt == NSUB - 1 and j == 3),
                            )
                nc.vector.tensor_tensor(
                    kv_acc[:, 4 * p : 4 * p + 4, :], kvp,
                    kv_acc[:, 4 * p : 4 * p + 4, :], AL.add)

            # ---- prep: weight DMA blocks interleaved with chunk-0/1 work ----
            def prep_w_blocks(src, nrows, dst, dst_off, scale):
                blocks = []
                nt = nrows // P
                c0 = 0
                while c0 < nt:
                    bs = min(3, nt - c0)
                    st = {}

                    def bdma(c0=c0, bs=bs, st=st):
                        wnat = wpool.tile([P, 3, DIM], F16, tag="wnat",
                                          name="wnat")[:]
                        nc.gpsimd.dma_start(
                            wnat[:, 0:bs, :],
                            src[c0 * P : (c0 + bs) * P, :].rearrange(
                                "(s p) k -> p s k", p=P))
                        st["wnat"] = wnat

                    def bcomp(c0=c0, bs=bs, st=st):
                        wnat = st["wnat"]
                        for kk in range(KT):
                            tp = work.tile([P, 512], F16, tag="work",
                                           name="ptp")[:]
                            for j in range(bs):
                                nc.tensor.transpose(
                                    tp[:, j * P : (j + 1) * P],
                                    wnat[:, j, kk * P : (kk + 1) * P], ident16)
                            hi = dst[:, kk, 0,
                                     dst_off + c0 * P : dst_off + (c0 + bs) * P]
                            nc.scalar.activation(hi, tp[:, 0 : bs * P], AF.Copy,
                                                 scale=scale)
                            nc.vector.scalar_tensor_tensor(
                                dst[:, kk, 1,
                                    dst_off + c0 * P : dst_off + (c0 + bs) * P],
                                tp[:, 0 : bs * P], scale, hi,
                                AL.mult, AL.subtract)

                    blocks.append((bdma, bcomp))
                    c0 += bs
                return blocks

            pm_st = {}

            def prep_pm_dma():
                pmn = wpool.tile([P, 2, D], F16, tag="pmn", name="pmn")[:]
                nc.gpsimd.dma_start(pmn, pm_d.rearrange("(s p) d -> p s d", p=P))
                pm_st["pmn"] = pmn

            def prep_pm():
                pmn = pm_st["pmn"]
                tp = work.tile([P, 512], F16, tag="work", name="ptp")[:]
                for s in range(2):
                    nc.tensor.transpose(tp[0:D, s * P : (s + 1) * P],
                                        pmn[:, s, :], ident16)
                nc.gpsimd.memset(pmbd, 0.0)
                nc.scalar.activation(pmbd[0:D, 0, 0:M], tp[0:D, 0:M], AF.Copy,
                                     scale=SPM * RATIO)
                nc.scalar.activation(pmbd[0:D, 1, M : 2 * M], tp[0:D, 0:M],
                                     AF.Copy, scale=SPM * RATIO)
                nc.scalar.activation(pmt2[0:D, :], tp[0:D, 0:M], AF.Copy,
                                     scale=RATIO)
                nc.scalar.activation(pmt2[D:P, :], tp[0:D, 0:M], AF.Copy,
                                     scale=RATIO)

            dma_x(0)
            wv = prep_w_blocks(qkvw_d[2 * DIM : 3 * DIM, :], DIM, wvhl, 0, SW)
            wqk_k = prep_w_blocks(qkvw_d[DIM : 2 * DIM, :], DIM, whl_qk,
                                  DIM, SW)
            wqk_q = prep_w_blocks(qkvw_d[0:DIM, :], DIM, whl_qk, 0, SW)
            wp = prep_w_blocks(projw_d, DIM, wphl, 0, SP)

            nc.gpsimd.memset(kv_acc, 0.0)
            blocks = wv + wqk_k + wqk_q + wp
            bst = {"dma": 0, "comp": 0}

            def bdma_next():
                if bst["dma"] < len(blocks):
                    blocks[bst["dma"]][0]()
                    bst["dma"] += 1

            def bcomp_next():
                if bst["comp"] < len(blocks):
                    blocks[bst["comp"]][1]()
                    bst["comp"] += 1
                    bdma_next()

            nop = lambda: None
            # phase A: wv + k-part of wqk; q-part and proj stream into chunk 0
            nA = len(wv) + len(wqk_k)
            fillers = {
                0: [lambda: transp_x(0, range(0, 3)),
                    (lambda: dma_x(1)) if 1 < NCH else nop],
                1: [lambda: transp_x(0, range(3, KT)), prep_pm],
                2: [lambda: emit_v(0, (0,)), lambda: emit_v(0, (1,)),
                    (lambda: transp_x(1, range(0, 3))) if 1 < NCH else nop],
                3: [lambda: emit_v(0, (2,)), lambda: emit_v(0, (3,)),
                    (lambda: transp_x(1, range(3, KT))) if 1 < NCH else nop,
                    (lambda: dma_x(2)) if 2 < NCH else nop],
            }
            bdma_next()
            prep_pm_dma()
            bdma_next()
            # PE p-state warmup: burn the cold DMA-wait ramping the clock
            wu = work.tile([P, 512], F16, tag="work", name="wu")[:]
            for _ in range(12):
                for s in range(4):
                    nc.tensor.transpose(wu[:, s * P : (s + 1) * P], ident16,
                                        ident16)
            for i in range(nA):
                for f in fillers.get(i, []):
                    f()
                bcomp_next()

            # ---- pass 1 main loop ----
            for ich in range(NCH):
                first = ich == 0
                pend_kt = None
                pend = None
                for p in range(NPAIR):
                    kt = emit_kT(ich, p)
                    if first:
                        # stream remaining weight blocks (q-part + proj)
                        bcomp_next()
                        if p >= 3:
                            emit_qT(0, p - 3)
                    else:
                        emit_qT(ich, p)
                    if pend is not None:
                        emit_kv(ich, pend[0], pend[1])
                        if ich == NCH - 1:
                            pp = pend[0]
                            nc.scalar.activation(
                                kvm.rearrange("p b j c -> p (b j) c")[
                                    :, 4 * pp : 4 * pp + 4, :],
                                kv_acc[:, 4 * pp : 4 * pp + 4, :],
                                AF.Copy, scale=1.0 / SKP)
                        pend = None
                    if pend_kt is not None:
                        pend = (pend_kt[0], emit_kp(pend_kt[0], pend_kt[1]))
                    pend_kt = (p, kt)
                    if ich == NCH - 1 and not first:
                        # chunk-0 q features computed here so pass 2 starts hot
                        emit_qp(0, (2 * p, 2 * p + 1), work, "work")
                    if p == 0 and 1 <= ich and ich + 1 < NCH:
                        transp_x(ich + 1, range(0, 3))
                    if p == 2 and 1 <= ich and ich + 1 < NCH:
                        transp_x(ich + 1, range(3, KT))
                    if p == 3 and 1 <= ich and ich + 2 < NCH:
                        dma_x(ich + 2)
                    if p >= 3 and ich + 1 < NCH:
                        emit_v(ich + 1, (p - 3,))
                if pend is not None:
                    emit_kv(ich, pend[0], pend[1])
                    if ich == NCH - 1:
                        pp = pend[0]
                        nc.scalar.activation(
                            kvm.rearrange("p b j c -> p (b j) c")[
                                :, 4 * pp : 4 * pp + 4, :],
                            kv_acc[:, 4 * pp : 4 * pp + 4, :],
                            AF.Copy, scale=1.0 / SKP)
                pend = (pend_kt[0], emit_kp(pend_kt[0], pend_kt[1]))
                if ich + 1 < NCH:
                    emit_v(ich + 1, (3,))
                emit_kv(ich, pend[0], pend[1])
                if ich == NCH - 1:
                    nc.scalar.activation(
                        kvm.rearrange("p b j c -> p (b j) c")[:, 20:24, :],
                        kv_acc[:, 20:24, :], AF.Copy, scale=1.0 / SKP)
                if first:
                    for p3 in range(3, NPAIR):
                        emit_qT(0, p3)
                    if NCH == 1:
                        emit_qp(0, range(H), work, "work")
                xhls.pop(ich)

    with tc.tile_pool(name="csps", bufs=1, space="PSUM") as cspool:
        css = []
        for b in range(4):
            cs = cspool.tile([1, 6, D + 1], F32, tag=f"cs{b}", name="cs")[:]
            for j in range(6):
                nc.tensor.matmul(cs[:, j, :], epscol, kvm[:, b, j, :],
                                 start=(j == 0), stop=(j == 5))
            css.append(cs)
        for b in range(4):
            nc.scalar.copy(kvmcs[:, b], css[b])

    # ---- pass 2 ----
    with tc.tile_pool(name="p2attn", bufs=2) as atp_sb, \
         tc.tile_pool(name="p2rd", bufs=2) as rdp, \
         tc.tile_pool(name="p2ahl", bufs=2) as ahlp, \
         tc.tile_pool(name="p2y", bufs=2) as yp, \
         tc.tile_pool(name="ps2qp", bufs=3, space="PSUM") as qppsum, \
         tc.tile_pool(name="ps2nm", bufs=2, space="PSUM") as numpsum, \
         tc.tile_pool(name="ps2at", bufs=1, space="PSUM") as atpsum, \
         tc.tile_pool(name="ps2y", bufs=1, space="PSUM") as ypsum:

        def emit_y(ich, ahl, lt):
            l0 = ich * LCH
            yps = ypsum.tile([P, DIM], F32, tag="yps", name="yps")[:]
            for c0, c1 in ((0, 512), (512, DIM)):
                b = (ones1, pb_row[:, c0:c1]) if has_proj_b else None
                _dr_gemm(nc, yps[:, c0:c1], wphl, ahl, (c0, c1), lt=lt, bias=b)
            ysb = yp.tile([P, DIM], F32, tag="ysb", name="ysb")[:]
            nc.scalar.activation(ysb, yps, AF.Copy, scale=iy)
            nc.sync.dma_start(y_d[l0 + lt * P : l0 + (lt + 1) * P, :], ysb)

        pend_y = []
        for ich in range(NCH):
            qp_sb = qp2[ich % 2]
            ahl = ahlp.tile([P, KT, 2, LCH], F8, tag="ahl", name="ahl")[:]
            for lt in range(NSUB):
                nmps = [numpsum.tile([P, 6, D + 1], F32, tag="nm", name="nmps")[:]
                        for _ in range(2)]
                for h in range(H):
                    g = h // 6
                    for mt in range(2):
                        ai = h * 2 + mt
                        if _QP_ENG[ai] == "act" and h % 6 != 0:
                            nc.tensor.matmul(
                                nmps[g][:, h % 6, :], ones1,
                                kvmcs[0:1, ai // 6, ai % 6, :],
                                start=False, stop=False)
                    for mt in range(2):
                        ai = h * 2 + mt
                        nc.tensor.matmul(
                            nmps[g][:, h % 6, :],
                            qp_sb[:, h, mt, lt * P : (lt + 1) * P],
                            kvm[:, ai // 6, ai % 6, :],
                            start=(mt == 0 and h % 6 == 0),
                            stop=(mt == 1 and h % 6 == 5))
                rd = rdp.tile([P, H], F32, tag="rd", name="rd")[:]
                attn = atp_sb.tile([P, H, D], F16, tag="attn", name="attn")[:]
                for g in range(2):
                    nc.vector.reciprocal(rd[:, g * 6 : (g + 1) * 6],
                                         nmps[g][:, :, D])
                    nc.vector.tensor_tensor(
                        attn[:, g * 6 : (g + 1) * 6, :],
                        nmps[g][:, :, 0:D],
                        rd[:, g * 6 : (g + 1) * 6, None].to_broadcast([P, 6, D]),
                        AL.mult)
                if ich + 1 < NCH:
                    emit_qp(ich + 1, range(3 * lt, 3 * lt + 3), qppsum, "qps")
                if len(pend_y) >= (2 if ich + 1 < NCH else 1):
                    emit_y(*pend_y.pop(0))
                atps = atpsum.tile([P, DIM], F16, tag="at", name="atps")[:]
                for kk in range(KT):
                    nc.tensor.transpose(atps[:, kk * P : (kk + 1) * P],
                                        attn[:, 2 * kk : 2 * kk + 2, :], ident16)
                a3 = atps.rearrange("p (k l) -> p k l", k=KT)
                nc.scalar.activation(ahl[:, :, 0, lt * P : (lt + 1) * P], a3,
                                     AF.Copy, scale=SA)
                nc.vector.scalar_tensor_tensor(
                    ahl[:, :, 1, lt * P : (lt + 1) * P], a3, SA,
                    ahl[:, :, 0, lt * P : (lt + 1) * P], AL.mult, AL.subtract)
                pend_y.append((ich, ahl, lt))
        for args in pend_y:
            emit_y(*args)


_CACHE = {}


def _get_nc(L=4096, hqb=False, hpb=False):
    key = ("nc", L, hqb, hpb)
    if key not in _CACHE:
        _CACHE[key] = build(L, hqb, hpb)
    return _CACHE[key]


last_exec_time_ns = None
last_profile = None


def kernel(x, qkv_w, qkv_b, proj_w, proj_b, proj_mat):
    global last_exec_time_ns, last_profile
    from concourse.bass_utils import run_bass_kernel_spmd

    x = np.asarray(x, np.float32)
    B, L, _ = x.shape
    hqb = bool(np.any(np.asarray(qkv_b)))
    hpb = bool(np.any(np.asarray(proj_b)))
    nc = _get_nc(L, hqb, hpb)
    base = {
        "qkv_w": np.ascontiguousarray(np.asarray(qkv_w, np.float32)),
        "qkv_b": np.ascontiguousarray(np.asarray(qkv_b, np.float32)),
        "proj_w": np.ascontiguousarray(np.asarray(proj_w, np.float32)),
        "proj_b": np.ascontiguousarray(np.asarray(proj_b, np.float32)),
        "proj_mat": np.ascontiguousarray(np.asarray(proj_mat, np.float32)),
    }
    in_maps = [dict(base, x=np.ascontiguousarray(x[b])) for b in range(B)]
    trace = bool(int(os.environ.get("KERNEL_TRACE", "0")))
    res = run_bass_kernel_spmd(nc, in_maps, core_ids=list(range(B)), trace=trace)
    last_exec_time_ns = res.exec_time_ns
    last_profile = res.profile_json
    return np.stack([res.results[b]["y"] for b in range(B)], axis=0)


def _ref_np(x, qkv_w, qkv_b, proj_w, proj_b, proj_mat):
    Ls = x.shape[0]
    qkv = x @ qkv_w.T + qkv_b
    qkv = qkv.reshape(Ls, 3, H, D)
    q, k, v = qkv[:, 0], qkv[:, 1], qkv[:, 2]
    qp = np.maximum(RATIO * np.einsum("lhd,md->lhm", q, proj_mat), 0) + EPS
    kp = np.maximum(RATIO * np.einsum("lhd,md->lhm", k, proj_mat), 0) + EPS
    kv = np.einsum("lhm,lhd->hmd", kp, v)
    ks = kp.sum(axis=0)
    num = np.einsum("lhm,hmd->lhd", qp, kv)
    den = np.einsum("lhm,hm->lh", qp, ks)
    out = (num / den[..., None]).reshape(Ls, DIM)
    return out @ proj_w.T + proj_b


if __name__ == "__main__":
    from concourse.bass_interp import CoreSim

    Ls = int(os.environ.get("SIM_L", "512"))
    use_bias = bool(int(os.environ.get("SIM_BIAS", "1")))
    rng = np.random.default_rng(0)
    x = rng.standard_normal((Ls, DIM), dtype=np.float32)
    qkv_w = (rng.standard_normal((3 * DIM, DIM), dtype=np.float32) * DIM**-0.5)
    qkv_b = (rng.standard_normal(3 * DIM, dtype=np.float32) * 0.1
             if use_bias else np.zeros(3 * DIM, np.float32))
    proj_w = (rng.standard_normal((DIM, DIM), dtype=np.float32) * DIM**-0.5)
    proj_b = (rng.standard_normal(DIM, dtype=np.float32) * 0.1
              if use_bias else np.zeros(DIM, np.float32))
    proj_mat = rng.standard_normal((M, D), dtype=np.float32)

    print(f"building L={Ls} bias={use_bias} ...")
    nc = build(Ls, use_bias, use_bias)
    print("simulating ...")
    sim = CoreSim(nc)
    for name, arr in [("x", x), ("qkv_w", qkv_w), ("qkv_b", qkv_b),
                      ("proj_w", proj_w), ("proj_b", proj_b),
                      ("proj_mat", proj_mat)]:
        sim.tensor(name)[:] = arr
    sim.simulate(check_with_hw=False)
    got = np.array(sim.tensor("y"))
    want = _ref_np(x, qkv_w, qkv_b, proj_w, proj_b, proj_mat)
    err = np.abs(got - want)
    rel = np.linalg.norm(got - want) / np.linalg.norm(want)
    print("max abs err:", err.max(), " rel fro err:", rel)
    assert rel < 2e-2, "sim mismatch"
    print("SIM OK")


# revision 17
# speedup vs baseline: 1.0281x; 1.0007x over previous
"""FAVOR+ (Performer) non-causal linear attention on 8 Trainium2 NeuronCores.

Sharding: data-parallel over batch B=8 -> one batch element per core.

Per-core pipeline (L=4096, DIM=768, H=12, D=64, M=256):
  prep : cast-DMA weights to fp16, PE-transpose to feature-major, split into
         fp8e4m3 hi/lo pairs (scaled) for DoubleRow matmuls; DMA blocks
         interleaved with chunk-0/1 transposes and v so PE never starves
  pass1: per 512-row chunk: cast-DMA x to fp16; PE-transpose -> xT hi/lo fp8;
         kT/qT/v via fp8 DoubleRow hi/lo-compensated GEMMs (4.5 cyc per
         128x512 output tile instead of 6); k features fp16 with blockdiag pm
         (2 heads per matmul); kv accumulated m-major [m,65] into 4 persistent
         PSUM banks across all chunks (ones-augmented v gives k_sum for
         free); qT staged in SBUF fp16 (no DRAM round trip).  Emission is
         software-pipelined: transposes/v run 2 chunks ahead; kv trails one
         pair behind its kp conversion.
  mid  : kv PSUM -> fp16 SBUF (already m-major); eps*colsum(kv) rows for the
         ACT-assigned q-feature heads
  pass2: q features m-major fp16 (computed one chunk ahead, spread across the
         l-tile loop; relu+eps on DVE/Pool, plain relu on ACT with the eps
         restored by a rank-1 matmul into num); num L-major [l,65] (den =
         col 64); reciprocal + broadcast multiply on DVE; attn PE-transposed
         to feature-major, split fp8 hi/lo; y via DoubleRow GEMM -> DMA out
"""

import math
import os
import sys
from contextlib import ExitStack

import numpy as np

for _p in ("/opt/trn_rl_repo",):
    if _p not in sys.path and os.path.isdir(_p):
        sys.path.insert(0, _p)

import concourse.bass as bass  # noqa: E402
import concourse.mybir as mybir  # noqa: E402
import concourse.tile as tile  # noqa: E402
from concourse import bacc  # noqa: E402

P = 128
DIM = 768
H = 12
D = 64
M = 256
KT = DIM // P   # 6 contraction c-tiles
NPAIR = H // 2  # 6 head pairs
EPS = 1e-3
RATIO = 1.0 / math.sqrt(float(M))

SX = 16.0    # x ~ N(0,1)
SW = 32.0    # qkv_w ~ N(0, 1/768)
SA = 64.0    # attn ~ 0.1
SP = 32.0    # proj_w ~ N(0, 1/768)
SKT = 16.0   # kT ~ N(0,1) -> fp8 for the k-feature GEMM
SPM = 32.0   # RATIO*pm ~ N(0, 1/256) -> fp8
SKP = SKT * SPM  # k-feature path runs in this scaled domain until kvm

F32 = mybir.dt.float32
F16 = mybir.dt.float16
F8 = mybir.dt.float8e4
AL = mybir.AluOpType
AF = mybir.ActivationFunctionType
DR = mybir.MatmulPerfMode.DoubleRow

# pass-2 q-feature conversion engine per (head, mtile) slot ai=h*2+mt.
# Slots of the group-starting heads (ai 0,1,12,13) stay off ACT so each num
# PSUM group's first instruction is its start matmul.
_QP_ENG = {}
_c = 0
for _ai in range(2 * H):
    if _ai in (0, 1, 12, 13):
        _QP_ENG[_ai] = "dve"
    else:
        _QP_ENG[_ai] = ("dve", "act", "act")[_c % 3]
        _c += 1


def build(L=4096, has_qkv_b=False, has_proj_b=False):
    LCH = 512
    NCH = L // LCH
    NSUB = LCH // P  # 4

    nc = bacc.Bacc("TRN2", target_bir_lowering=False, debug=False)
    x_d = nc.dram_tensor("x", [L, DIM], F32, kind="ExternalInput").ap()
    qkvw_d = nc.dram_tensor("qkv_w", [3 * DIM, DIM], F32, kind="ExternalInput").ap()
    qkvb_d = nc.dram_tensor("qkv_b", [3 * DIM], F32, kind="ExternalInput").ap()
    projw_d = nc.dram_tensor("proj_w", [DIM, DIM], F32, kind="ExternalInput").ap()
    projb_d = nc.dram_tensor("proj_b", [DIM], F32, kind="ExternalInput").ap()
    pm_d = nc.dram_tensor("proj_mat", [M, D], F32, kind="ExternalInput").ap()
    y_d = nc.dram_tensor("y", [L, DIM], F32, kind="ExternalOutput").ap()

    with tile.TileContext(nc) as tc:
        with ExitStack() as ctx:
            _body(ctx, tc, x_d, qkvw_d, qkvb_d, projw_d, projb_d, pm_d, y_d,
                  L, LCH, NCH, NSUB, has_qkv_b, has_proj_b)
    nc.compile()
    return nc


def _dr_gemm(nc, out, whl, xhl, wcols, lt=None, bias=None):
    """Accumulating fp8 DoubleRow hi/lo-compensated GEMM over 768 contraction.

    whl/xhl: [128, KT, 2, *] fp8 with hi at [:,kk,0,:], lo at [:,kk,1,:].
    lt=None : out[wc, l]; stationary = whl cols wcols, moving = xhl  (kT/qT)
    lt given: out[l, wc]; stationary = xhl l-tile cols, moving = whl (v/y)
    """
    c0, c1 = wcols
    n = c1 - c0
    if lt is None:
        for i in range(KT // 2):
            for kk in (2 * i, 2 * i + 1):
                nc.tensor.matmul(
                    out, whl[:, kk, :, c0:c1],
                    xhl[:, kk, 0:1, :].to_broadcast([P, 2, out.shape[-1]]),
                    start=(kk == 0), stop=False, perf_mode=DR,
                )
            nc.tensor.matmul(
                out, whl[:, 2 * i : 2 * i + 2, 0, c0:c1],
                xhl[:, 2 * i : 2 * i + 2, 1, :],
                start=False, stop=(bias is None and i == KT // 2 - 1),
                perf_mode=DR,
            )
    else:
        l0 = lt * P
        for i in range(KT // 2):
            for kk in (2 * i, 2 * i + 1):
                nc.tensor.matmul(
                    out, xhl[:, kk, :, l0 : l0 + P],
                    whl[:, kk, 0:1, c0:c1].to_broadcast([P, 2, n]),
                    start=(kk == 0), stop=False, perf_mode=DR,
                )
            nc.tensor.matmul(
                out, xhl[:, 2 * i : 2 * i + 2, 0, l0 : l0 + P],
                whl[:, 2 * i : 2 * i + 2, 1, c0:c1],
                start=False, stop=(bias is None and i == KT // 2 - 1),
                perf_mode=DR,
            )
    if bias is not None:
        ones_row, brow = bias
        nc.tensor.matmul(out, ones_row, brow, start=False, stop=True)


def _body(ctx, tc, x_d, qkvw_d, qkvb_d, projw_d, projb_d, pm_d, y_d,
          L, LCH, NCH, NSUB, has_qkv_b, has_proj_b):
    nc = tc.nc
    iqkv = 1.0 / (SX * SW)
    iy = 1.0 / (SA * SP)

    persist = ctx.enter_context(tc.tile_pool(name="persist", bufs=1))

    ident16 = persist.tile([P, P], F16, tag="ident16", name="ident16")[:]
    nc.gpsimd.memset(ident16, 0.0)
    nc.gpsimd.affine_select(
        out=ident16, in_=ident16, compare_op=AL.not_equal, fill=1.0,
        base=0, pattern=[[-1, P]], channel_multiplier=1,
    )
    ones1 = persist.tile([1, P], F16, tag="ones1", name="ones1")[:]
    nc.gpsimd.memset(ones1, 1.0)
    epscol = persist.tile([P, 1], F16, tag="epscol", name="epscol")[:]
    nc.gpsimd.memset(epscol, EPS)
    epsb = persist.tile([P, 1], F32, tag="epsb", name="epsb")[:]
    nc.gpsimd.memset(epsb, SKP * EPS)

    whl_qk = persist.tile([P, KT, 2, 2 * DIM], F8, tag="whl_qk", name="whl_qk")[:]
    wvhl = persist.tile([P, KT, 2, DIM], F8, tag="wvhl", name="wvhl")[:]
    wphl = persist.tile([P, KT, 2, DIM], F8, tag="wphl", name="wphl")[:]
    # folded blockdiag pm for the fp8 DoubleRow k-feature GEMM:
    # slice 0 = [RATIO*pmT | 0] (c rows 0:64), slice 1 = [0 | RATIO*pmT]
    pmbd = persist.tile([P, 2, 2 * M], F8, tag="pmbd", name="pmbd")[:]
    pmt2 = persist.tile([P, M], F16, tag="pmt2", name="pmt2")[:]
    qt_sb = persist.tile([P, NPAIR, L], F16, tag="qt_sb", name="qt_sb")[:]
    kvm = persist.tile([P, 4, 6, D + 1], F16, tag="kvm", name="kvm")[:]
    kvmcs = persist.tile([1, 4, 6, D + 1], F16, tag="kvmcs", name="kvmcs")[:]

    if has_qkv_b:
        qkb = persist.tile([P, 2 * KT], F32, tag="qkb", name="qkb")[:]
        nc.sync.dma_start(qkb, qkvb_d.rearrange("(t p) -> p t", p=P)[:, 0 : 2 * KT])
        qkbk = persist.tile([P, KT], F32, tag="qkbk", name="qkbk")[:]
        nc.scalar.activation(qkbk, qkb[:, KT : 2 * KT], AF.Copy, scale=SKT)
        vbf = persist.tile([1, DIM], F32, tag="vbf", name="vbf")[:]
        nc.sync.dma_start(vbf, qkvb_d[2 * DIM : 3 * DIM].unsqueeze(0))
        vb_row = persist.tile([1, DIM], F16, tag="vb_row", name="vb_row")[:]
        nc.scalar.activation(vb_row, vbf, AF.Copy, scale=SX * SW)
    if has_proj_b:
        pbf = persist.tile([1, DIM], F32, tag="pbf", name="pbf")[:]
        nc.sync.dma_start(pbf, projb_d.unsqueeze(0))
        pb_row = persist.tile([1, DIM], F16, tag="pb_row", name="pb_row")[:]
        nc.scalar.activation(pb_row, pbf, AF.Copy, scale=SA * SP)

    vsb = persist.tile([P, 2, NSUB, H, D + 1], F16, tag="vsb", name="vsb")[:]
    nc.gpsimd.memset(vsb[:, :, :, :, D : D + 1], 1.0)

    # kv accumulator in SBUF fp32 (indexed by ai = h*2+mt)
    kv_acc = persist.tile([P, 2 * H, D + 1], F32, tag="kv_acc", name="kv_acc")[:]

    # pass-2 q-feature tiles, double-buffered by chunk parity
    qp2 = [persist.tile([P, H, 2, LCH], F16, tag=f"qp2_{i}", name=f"qp2_{i}")[:]
           for i in range(2)]

    def emit_qp(ich, heads, psum_pool, tag):
        l0 = ich * LCH
        qp_sb = qp2[ich % 2]
        for h in heads:
            p, h2 = h // 2, h % 2
            for mt in range(2):
                qps = psum_pool.tile([P, LCH], F32, tag=tag, name="qps")[:]
                nc.tensor.matmul(
                    qps,
                    pmt2[h2 * D : (h2 + 1) * D, mt * P : (mt + 1) * P],
                    qt_sb[h2 * D : (h2 + 1) * D, p, l0 : l0 + LCH],
                    start=True, stop=True)
                eng = _QP_ENG[h * 2 + mt]
                if eng == "act":
                    # plain relu; eps restored via rank-1 kvmcs in num
                    nc.scalar.activation(qp_sb[:, h, mt, :], qps, AF.Relu)
                else:
                    nc.vector.tensor_scalar(qp_sb[:, h, mt, :], qps,
                                            EPS, EPS, AL.add, AL.max)

    if True:
        with tc.tile_pool(name="p1x", bufs=2) as xp, \
             tc.tile_pool(name="p1xhl", bufs=2) as xhlp, \
             tc.tile_pool(name="p1kt", bufs=3) as ktp, \
             tc.tile_pool(name="p1kp", bufs=8) as kpp, \
             tc.tile_pool(name="wprep", bufs=3) as wpool, \
             tc.tile_pool(name="p1work", bufs=8, space="PSUM") as work:

            xnats = {}
            xhls = {}

            def dma_x(ich):
                l0 = ich * LCH
                xnat = xp.tile([P, NSUB, DIM], F16, tag="xnat", name="xnat")[:]
                nc.gpsimd.dma_start(
                    xnat,
                    x_d[l0 : l0 + LCH, :].rearrange("(s p) k -> p s k", p=P))
                xnats[ich] = xnat

            xhl_cur = {}

            def transp_x(ich, kks=range(KT)):
                if ich in xhl_cur:
                    xnat, xhl = xhl_cur[ich]
                else:
                    xnat = xnats.pop(ich)
                    xhl = xhlp.tile([P, KT, 2, LCH], F8, tag="xhl",
                                    name="xhl")[:]
                    xhl_cur[ich] = (xnat, xhl)
                for kk in kks:
                    tp = work.tile([P, 512], F16, tag="work", name="ttp")[:]
                    for s in range(NSUB):
                        nc.tensor.transpose(
                            tp[:, s * P : (s + 1) * P],
                            xnat[:, s, kk * P : (kk + 1) * P], ident16)
                    nc.scalar.activation(xhl[:, kk, 0, :], tp, AF.Copy,
                                         scale=SX)
                    nc.vector.scalar_tensor_tensor(
                        xhl[:, kk, 1, :], tp, SX, xhl[:, kk, 0, :],
                        AL.mult, AL.subtract)
                xhls[ich] = xhl

            def emit_v(ich, subs):
                vs = ich % 2
                xhl = xhls[ich]
                for s in subs:
                    for ci in range(2):
                        c0 = ci * 384
                        vps = work.tile([P, 512], F32, tag="work", name="vps")[:]
                        bias = None
                        if has_qkv_b:
                            bias = (ones1, vb_row[:, c0 : c0 + 384])
                        _dr_gemm(nc, vps[:, 0:384], wvhl, xhl, (c0, c0 + 384),
                                 lt=s, bias=bias)
                        nc.scalar.activation(
                            vsb[:, vs, s, 6 * ci : 6 * ci + 6, 0:D],
                            vps[:, 0:384].rearrange("p (h d) -> p h d", d=D),
                            AF.Copy, scale=iqkv)

            def emit_kT(ich, p):
                ktps = work.tile([P, 512], F32, tag="work", name="ktps")[:]
                _dr_gemm(nc, ktps, whl_qk, xhls[ich],
                         (DIM + p * P, DIM + (p + 1) * P))
                # fold [128,512] -> [64, 2, 512] fp8 (scaled) for DoubleRow
                kt = ktp.tile([P, 2, LCH], F8, tag="kt", name="kt")[:]
                for h2 in range(2):
                    if has_qkv_b:
                        nc.scalar.activation(
                            kt[0:D, h2, :], ktps[h2 * D : (h2 + 1) * D, :],
                            AF.Identity,
                            bias=qkbk[h2 * D : (h2 + 1) * D, p : p + 1],
                            scale=SKT * iqkv)
                    else:
                        nc.scalar.activation(
                            kt[0:D, h2, :], ktps[h2 * D : (h2 + 1) * D, :],
                            AF.Copy, scale=SKT * iqkv)
                return kt

            def emit_kp(p, kt):
                kps = []
                for lt in range(NSUB):
                    kpps = work.tile([P, 512], F32, tag="work", name="kpps")[:]
                    nc.tensor.matmul(kpps,
                                     kt[0:D, :, lt * P : (lt + 1) * P],
                                     pmbd[0:D], start=True, stop=True,
                                     perf_mode=DR)
                    kp = kpp.tile([P, 2 * M], F16, tag="kp", name="kp")[:]
                    # k-feature path is scaled by SKP; it cancels in num/den.
                    # ACT-assigned tiles use relu(z+eps) ~ relu(z)+eps
                    # (differs by <= eps only for z in (-eps, 0)); exact
                    # max(z+eps, eps) on DVE for the rest
                    if lt == 2:
                        nc.scalar.activation(kp, kpps, AF.Relu, bias=epsb)
                    else:
                        nc.vector.tensor_scalar(kp, kpps, SKP * EPS, SKP * EPS,
                                                AL.add, AL.max)
                    kps.append(kp)
                return kps

            def emit_qT(ich, p):
                l0 = ich * LCH
                qtps = work.tile([P, 512], F32, tag="work", name="qtps")[:]
                _dr_gemm(nc, qtps, whl_qk, xhls[ich], (p * P, (p + 1) * P))
                if has_qkv_b:
                    nc.scalar.activation(qt_sb[:, p, l0 : l0 + LCH], qtps,
                                         AF.Identity,
                                         bias=qkb[:, p : p + 1], scale=iqkv)
                else:
                    nc.scalar.activation(qt_sb[:, p, l0 : l0 + LCH], qtps,
                                         AF.Copy, scale=iqkv)

            def emit_kv(ich, p, kps):
                vs = ich % 2
                kvp = work.tile([P, 4, D + 1], F32, tag="work", name="kvp")[:]
                for lt in range(NSUB):
                    kp = kps[lt]
                    for h2 in range(2):
                        h = 2 * p + h2
                        for mt in range(2):
                            j = h2 * 2 + mt
                            nc.tensor.matmul(
                                kvp[:, j, :],
                                kp[:, j * P : (j + 1) * P],
                                vsb[:, vs, lt, h, :],
                                start=(lt == 0 and j == 0),
                                stop=(lt == NSUB - 1 and j == 3),
                            )
                nc.vector.tensor_tensor(
                    kv_acc[:, 4 * p : 4 * p + 4, :], kvp,
                    kv_acc[:, 4 * p : 4 * p + 4, :], AL.add)

            # ---- prep: weight DMA blocks interleaved with chunk-0/1 work ----
            def prep_w_blocks(src, nrows, dst, dst_off, scale):
                blocks = []
                nt = nrows // P
                c0 = 0
                while c0 < nt:
                    bs = min(3, nt - c0)
                    st = {}

                    def bdma(c0=c0, bs=bs, st=st):
                        wnat = wpool.tile([P, 3, DIM], F16, tag="wnat",
                                          name="wnat")[:]
                        nc.gpsimd.dma_start(
                            wnat[:, 0:bs, :],
                            src[c0 * P : (c0 + bs) * P, :].rearrange(
                                "(s p) k -> p s k", p=P))
                        st["wnat"] = wnat

                    def bcomp(c0=c0, bs=bs, st=st):
                        wnat = st["wnat"]
                        for kk in range(KT):
                            tp = work.tile([P, 512], F16, tag="work",
                                           name="ptp")[:]
                            for j in range(bs):
                                nc.tensor.transpose(
                                    tp[:, j * P : (j + 1) * P],
                                    wnat[:, j, kk * P : (kk + 1) * P], ident16)
                            hi = dst[:, kk, 0,
                                     dst_off + c0 * P : dst_off + (c0 + bs) * P]
                            nc.scalar.activation(hi, tp[:, 0 : bs * P], AF.Copy,
                                                 scale=scale)
                            nc.vector.scalar_tensor_tensor(
                                dst[:, kk, 1,
                                    dst_off + c0 * P : dst_off + (c0 + bs) * P],
                                tp[:, 0 : bs * P], scale, hi,
                                AL.mult, AL.subtract)

                    blocks.append((bdma, bcomp))
                    c0 += bs
                return blocks

            pm_st = {}

            def prep_pm_dma():
                pmn = wpool.tile([P, 2, D], F16, tag="pmn", name="pmn")[:]
                nc.gpsimd.dma_start(pmn, pm_d.rearrange("(s p) d -> p s d", p=P))
                pm_st["pmn"] = pmn

            def prep_pm():
                pmn = pm_st["pmn"]
                tp = work.tile([P, 512], F16, tag="work", name="ptp")[:]
                for s in range(2):
                    nc.tensor.transpose(tp[0:D, s * P : (s + 1) * P],
                                        pmn[:, s, :], ident16)
                nc.gpsimd.memset(pmbd, 0.0)
                nc.scalar.activation(pmbd[0:D, 0, 0:M], tp[0:D, 0:M], AF.Copy,
                                     scale=SPM * RATIO)
                nc.scalar.activation(pmbd[0:D, 1, M : 2 * M], tp[0:D, 0:M],
                                     AF.Copy, scale=SPM * RATIO)
                nc.scalar.activation(pmt2[0:D, :], tp[0:D, 0:M], AF.Copy,
                                     scale=RATIO)
                nc.scalar.activation(pmt2[D:P, :], tp[0:D, 0:M], AF.Copy,
                                     scale=RATIO)

            dma_x(0)
            wv = prep_w_blocks(qkvw_d[2 * DIM : 3 * DIM, :], DIM, wvhl, 0, SW)
            wqk_k = prep_w_blocks(qkvw_d[DIM : 2 * DIM, :], DIM, whl_qk,
                                  DIM, SW)
            wqk_q = prep_w_blocks(qkvw_d[0:DIM, :], DIM, whl_qk, 0, SW)
            wp = prep_w_blocks(projw_d, DIM, wphl, 0, SP)

            nc.gpsimd.memset(kv_acc, 0.0)
            blocks = wv + wqk_k + wqk_q + wp
            bst = {"dma": 0, "comp": 0}

            def bdma_next():
                if bst["dma"] < len(blocks):
                    blocks[bst["dma"]][0]()
                    bst["dma"] += 1

            def bcomp_next():
                if bst["comp"] < len(blocks):
                    blocks[bst["comp"]][1]()
                    bst["comp"] += 1
                    bdma_next()

            nop = lambda: None
            # phase A: wv + k-part of wqk; q-part and proj stream into chunk 0
            nA = len(wv) + len(wqk_k)
            fillers = {
                0: [lambda: transp_x(0, range(0, 3)),
                    (lambda: dma_x(1)) if 1 < NCH else nop],
                1: [lambda: transp_x(0, range(3, KT)), prep_pm],
                2: [lambda: emit_v(0, (0,)), lambda: emit_v(0, (1,)),
                    (lambda: transp_x(1, range(0, 3))) if 1 < NCH else nop],
                3: [lambda: emit_v(0, (2,)), lambda: emit_v(0, (3,)),
                    (lambda: transp_x(1, range(3, KT))) if 1 < NCH else nop,
                    (lambda: dma_x(2)) if 2 < NCH else nop],
            }
            bdma_next()
            prep_pm_dma()
            bdma_next()
            # PE p-state warmup: burn the cold DMA-wait ramping the clock
            wu = work.tile([P, 512], F16, tag="work", name="wu")[:]
            for _ in range(12):
                for s in range(4):
                    nc.tensor.transpose(wu[:, s * P : (s + 1) * P], ident16,
                                        ident16)
            for i in range(nA):
                for f in fillers.get(i, []):
                    f()
                bcomp_next()

            # ---- pass 1 main loop ----
            for ich in range(NCH):
                first = ich == 0
                pend_kt = None
                pend = None
                for p in range(NPAIR):
                    kt = emit_kT(ich, p)
                    if first:
                        # stream remaining weight blocks (q-part + proj)
                        bcomp_next()
                        if p >= 3:
                            emit_qT(0, p - 3)
                    else:
                        emit_qT(ich, p)
                    if pend is not None:
                        emit_kv(ich, pend[0], pend[1])
                        if ich == NCH - 1:
                            pp = pend[0]
                            nc.scalar.activation(
                                kvm.rearrange("p b j c -> p (b j) c")[
                                    :, 4 * pp : 4 * pp + 4, :],
                                kv_acc[:, 4 * pp : 4 * pp + 4, :],
                                AF.Copy, scale=1.0 / SKP)
                        pend = None
                    if pend_kt is not None:
                        pend = (pend_kt[0], emit_kp(pend_kt[0], pend_kt[1]))
                    pend_kt = (p, kt)
                    if ich == NCH - 1 and not first:
                        # chunk-0 q features computed here so pass 2 starts hot
                        emit_qp(0, (2 * p, 2 * p + 1), work, "work")
                    if p == 0 and 1 <= ich and ich + 1 < NCH:
                        transp_x(ich + 1, range(0, 4))
                    if p == 2 and 1 <= ich and ich + 1 < NCH:
                        transp_x(ich + 1, range(4, KT))
                    if p == 3 and 1 <= ich and ich + 2 < NCH:
                        dma_x(ich + 2)
                    if p >= 3 and ich + 1 < NCH:
                        emit_v(ich + 1, (p - 3,))
                if pend is not None:
                    emit_kv(ich, pend[0], pend[1])
                    if ich == NCH - 1:
                        pp = pend[0]
                        nc.scalar.activation(
                            kvm.rearrange("p b j c -> p (b j) c")[
                                :, 4 * pp : 4 * pp + 4, :],
                            kv_acc[:, 4 * pp : 4 * pp + 4, :],
                            AF.Copy, scale=1.0 / SKP)
                pend = (pend_kt[0], emit_kp(pend_kt[0], pend_kt[1]))
                if ich + 1 < NCH:
                    emit_v(ich + 1, (3,))
                emit_kv(ich, pend[0], pend[1])
                if ich == NCH - 1:
                    nc.scalar.activation(
                        kvm.rearrange("p b j c -> p (b j) c")[:, 20:24, :],
                        kv_acc[:, 20:24, :], AF.Copy, scale=1.0 / SKP)
                if first:
                    for p3 in range(3, NPAIR):
                        emit_qT(0, p3)
                    if NCH == 1:
                        emit_qp(0, range(H), work, "work")
                xhls.pop(ich)

    with tc.tile_pool(name="csps", bufs=1, space="PSUM") as cspool:
        css = []
        for b in range(4):
            cs = cspool.tile([1, 6, D + 1], F32, tag=f"cs{b}", name="cs")[:]
            for j in range(6):
                nc.tensor.matmul(cs[:, j, :], epscol, kvm[:, b, j, :],
                                 start=(j == 0), stop=(j == 5))
            css.append(cs)
        for b in range(4):
            nc.scalar.copy(kvmcs[:, b], css[b])

    # ---- pass 2 ----
    with tc.tile_pool(name="p2attn", bufs=2) as atp_sb, \
         tc.tile_pool(name="p2rd", bufs=2) as rdp, \
         tc.tile_pool(name="p2ahl", bufs=2) as ahlp, \
         tc.tile_pool(name="p2y", bufs=2) as yp, \
         tc.tile_pool(name="ps2qp", bufs=3, space="PSUM") as qppsum, \
         tc.tile_pool(name="ps2nm", bufs=2, space="PSUM") as numpsum, \
         tc.tile_pool(name="ps2at", bufs=1, space="PSUM") as atpsum, \
         tc.tile_pool(name="ps2y", bufs=1, space="PSUM") as ypsum:

        def emit_y(ich, ahl, lt):
            l0 = ich * LCH
            yps = ypsum.tile([P, DIM], F32, tag="yps", name="yps")[:]
            for c0, c1 in ((0, 512), (512, DIM)):
                b = (ones1, pb_row[:, c0:c1]) if has_proj_b else None
                _dr_gemm(nc, yps[:, c0:c1], wphl, ahl, (c0, c1), lt=lt, bias=b)
            ysb = yp.tile([P, DIM], F32, tag="ysb", name="ysb")[:]
            nc.scalar.activation(ysb, yps, AF.Copy, scale=iy)
            nc.sync.dma_start(y_d[l0 + lt * P : l0 + (lt + 1) * P, :], ysb)

        pend_y = []
        for ich in range(NCH):
            qp_sb = qp2[ich % 2]
            ahl = ahlp.tile([P, KT, 2, LCH], F8, tag="ahl", name="ahl")[:]
            for lt in range(NSUB):
                nmps = [numpsum.tile([P, 6, D + 1], F32, tag="nm", name="nmps")[:]
                        for _ in range(2)]
                for h in range(H):
                    g = h // 6
                    for mt in range(2):
                        ai = h * 2 + mt
                        if _QP_ENG[ai] == "act" and h % 6 != 0:
                            nc.tensor.matmul(
                                nmps[g][:, h % 6, :], ones1,
                                kvmcs[0:1, ai // 6, ai % 6, :],
                                start=False, stop=False)
                    for mt in range(2):
                        ai = h * 2 + mt
                        nc.tensor.matmul(
                            nmps[g][:, h % 6, :],
                            qp_sb[:, h, mt, lt * P : (lt + 1) * P],
                            kvm[:, ai // 6, ai % 6, :],
                            start=(mt == 0 and h % 6 == 0),
                            stop=(mt == 1 and h % 6 == 5))
                rd = rdp.tile([P, H], F32, tag="rd", name="rd")[:]
                attn = atp_sb.tile([P, H, D], F16, tag="attn", name="attn")[:]
                for g in range(2):
                    nc.vector.reciprocal(rd[:, g * 6 : (g + 1) * 6],
                                         nmps[g][:, :, D])
                    nc.vector.tensor_tensor(
                        attn[:, g * 6 : (g + 1) * 6, :],
                        nmps[g][:, :, 0:D],
                        rd[:, g * 6 : (g + 1) * 6, None].to_broadcast([P, 6, D]),
                        AL.mult)
                if ich + 1 < NCH:
                    emit_qp(ich + 1, range(3 * lt, 3 * lt + 3), qppsum, "qps")
                if len(pend_y) >= (2 if ich + 1 < NCH else 1):
                    emit_y(*pend_y.pop(0))
                atps = atpsum.tile([P, DIM], F16, tag="at", name="atps")[:]
                for kk in range(KT):
                    nc.tensor.transpose(atps[:, kk * P : (kk + 1) * P],
                                        attn[:, 2 * kk : 2 * kk + 2, :], ident16)
                a3 = atps.rearrange("p (k l) -> p k l", k=KT)
                nc.scalar.activation(ahl[:, :, 0, lt * P : (lt + 1) * P], a3,
                                     AF.Copy, scale=SA)
                nc.vector.scalar_tensor_tensor(
                    ahl[:, :, 1, lt * P : (lt + 1) * P], a3, SA,
                    ahl[:, :, 0, lt * P : (lt + 1) * P], AL.mult, AL.subtract)
                pend_y.append((ich, ahl, lt))
        for args in pend_y:
            emit_y(*args)


_CACHE = {}


def _get_nc(L=4096, hqb=False, hpb=False):
    key = ("nc", L, hqb, hpb)
    if key not in _CACHE:
        _CACHE[key] = build(L, hqb, hpb)
    return _CACHE[key]


last_exec_time_ns = None
last_profile = None


def kernel(x, qkv_w, qkv_b, proj_w, proj_b, proj_mat):
    global last_exec_time_ns, last_profile
    from concourse.bass_utils import run_bass_kernel_spmd

    x = np.asarray(x, np.float32)
    B, L, _ = x.shape
    hqb = bool(np.any(np.asarray(qkv_b)))
    hpb = bool(np.any(np.asarray(proj_b)))
    nc = _get_nc(L, hqb, hpb)
    base = {
        "qkv_w": np.ascontiguousarray(np.asarray(qkv_w, np.float32)),
        "qkv_b": np.ascontiguousarray(np.asarray(qkv_b, np.float32)),
        "proj_w": np.ascontiguousarray(np.asarray(proj_w, np.float32)),
        "proj_b": np.ascontiguousarray(np.asarray(proj_b, np.float32)),
        "proj_mat": np.ascontiguousarray(np.asarray(proj_mat, np.float32)),
    }
    in_maps = [dict(base, x=np.ascontiguousarray(x[b])) for b in range(B)]
    trace = bool(int(os.environ.get("KERNEL_TRACE", "0")))
    res = run_bass_kernel_spmd(nc, in_maps, core_ids=list(range(B)), trace=trace)
    last_exec_time_ns = res.exec_time_ns
    last_profile = res.profile_json
    return np.stack([res.results[b]["y"] for b in range(B)], axis=0)


def _ref_np(x, qkv_w, qkv_b, proj_w, proj_b, proj_mat):
    Ls = x.shape[0]
    qkv = x @ qkv_w.T + qkv_b
    qkv = qkv.reshape(Ls, 3, H, D)
    q, k, v = qkv[:, 0], qkv[:, 1], qkv[:, 2]
    qp = np.maximum(RATIO * np.einsum("lhd,md->lhm", q, proj_mat), 0) + EPS
    kp = np.maximum(RATIO * np.einsum("lhd,md->lhm", k, proj_mat), 0) + EPS
    kv = np.einsum("lhm,lhd->hmd", kp, v)
    ks = kp.sum(axis=0)
    num = np.einsum("lhm,hmd->lhd", qp, kv)
    den = np.einsum("lhm,hm->lh", qp, ks)
    out = (num / den[..., None]).reshape(Ls, DIM)
    return out @ proj_w.T + proj_b


if __name__ == "__main__":
    from concourse.bass_interp import CoreSim

    Ls = int(os.environ.get("SIM_L", "512"))
    use_bias = bool(int(os.environ.get("SIM_BIAS", "1")))
    rng = np.random.default_rng(0)
    x = rng.standard_normal((Ls, DIM), dtype=np.float32)
    qkv_w = (rng.standard_normal((3 * DIM, DIM), dtype=np.float32) * DIM**-0.5)
    qkv_b = (rng.standard_normal(3 * DIM, dtype=np.float32) * 0.1
             if use_bias else np.zeros(3 * DIM, np.float32))
    proj_w = (rng.standard_normal((DIM, DIM), dtype=np.float32) * DIM**-0.5)
    proj_b = (rng.standard_normal(DIM, dtype=np.float32) * 0.1
              if use_bias else np.zeros(DIM, np.float32))
    proj_mat = rng.standard_normal((M, D), dtype=np.float32)

    print(f"building L={Ls} bias={use_bias} ...")
    nc = build(Ls, use_bias, use_bias)
    print("simulating ...")
    sim = CoreSim(nc)
    for name, arr in [("x", x), ("qkv_w", qkv_w), ("qkv_b", qkv_b),
                      ("proj_w", proj_w), ("proj_b", proj_b),
                      ("proj_mat", proj_mat)]:
        sim.tensor(name)[:] = arr
    sim.simulate(check_with_hw=False)
    got = np.array(sim.tensor("y"))
    want = _ref_np(x, qkv_w, qkv_b, proj_w, proj_b, proj_mat)
    err = np.abs(got - want)
    rel = np.linalg.norm(got - want) / np.linalg.norm(want)
    print("max abs err:", err.max(), " rel fro err:", rel)
    assert rel < 2e-2, "sim mismatch"
    print("SIM OK")


# revision 18
# speedup vs baseline: 1.0283x; 1.0002x over previous
"""FAVOR+ (Performer) non-causal linear attention on 8 Trainium2 NeuronCores.

Sharding: data-parallel over batch B=8 -> one batch element per core.

Per-core pipeline (L=4096, DIM=768, H=12, D=64, M=256):
  prep : cast-DMA weights to fp16, PE-transpose to feature-major, split into
         fp8e4m3 hi/lo pairs (scaled) for DoubleRow matmuls; DMA blocks
         interleaved with chunk-0/1 transposes and v so PE never starves
  pass1: per 512-row chunk: cast-DMA x to fp16; PE-transpose -> xT hi/lo fp8;
         kT/qT/v via fp8 DoubleRow hi/lo-compensated GEMMs (4.5 cyc per
         128x512 output tile instead of 6); k features fp16 with blockdiag pm
         (2 heads per matmul); kv accumulated m-major [m,65] into 4 persistent
         PSUM banks across all chunks (ones-augmented v gives k_sum for
         free); qT staged in SBUF fp16 (no DRAM round trip).  Emission is
         software-pipelined: transposes/v run 2 chunks ahead; kv trails one
         pair behind its kp conversion.
  mid  : kv PSUM -> fp16 SBUF (already m-major); eps*colsum(kv) rows for the
         ACT-assigned q-feature heads
  pass2: q features m-major fp16 (computed one chunk ahead, spread across the
         l-tile loop; relu+eps on DVE/Pool, plain relu on ACT with the eps
         restored by a rank-1 matmul into num); num L-major [l,65] (den =
         col 64); reciprocal + broadcast multiply on DVE; attn PE-transposed
         to feature-major, split fp8 hi/lo; y via DoubleRow GEMM -> DMA out
"""

import math
import os
import sys
from contextlib import ExitStack

import numpy as np

for _p in ("/opt/trn_rl_repo",):
    if _p not in sys.path and os.path.isdir(_p):
        sys.path.insert(0, _p)

import concourse.bass as bass  # noqa: E402
import concourse.mybir as mybir  # noqa: E402
import concourse.tile as tile  # noqa: E402
from concourse import bacc  # noqa: E402

P = 128
DIM = 768
H = 12
D = 64
M = 256
KT = DIM // P   # 6 contraction c-tiles
NPAIR = H // 2  # 6 head pairs
EPS = 1e-3
RATIO = 1.0 / math.sqrt(float(M))

SX = 16.0    # x ~ N(0,1)
SW = 32.0    # qkv_w ~ N(0, 1/768)
SA = 64.0    # attn ~ 0.1
SP = 32.0    # proj_w ~ N(0, 1/768)
SKT = 16.0   # kT ~ N(0,1) -> fp8 for the k-feature GEMM
SPM = 32.0   # RATIO*pm ~ N(0, 1/256) -> fp8
SKP = SKT * SPM  # k-feature path runs in this scaled domain until kvm

F32 = mybir.dt.float32
F16 = mybir.dt.float16
F8 = mybir.dt.float8e4
AL = mybir.AluOpType
AF = mybir.ActivationFunctionType
DR = mybir.MatmulPerfMode.DoubleRow

# pass-2 q-feature conversion engine per (head, mtile) slot ai=h*2+mt.
# Slots of the group-starting heads (ai 0,1,12,13) stay off ACT so each num
# PSUM group's first instruction is its start matmul.
_QP_ENG = {}
_c = 0
for _ai in range(2 * H):
    if _ai in (0, 1, 12, 13):
        _QP_ENG[_ai] = "dve"
    else:
        _QP_ENG[_ai] = ("dve", "act", "act")[_c % 3]
        _c += 1


def build(L=4096, has_qkv_b=False, has_proj_b=False):
    LCH = 512
    NCH = L // LCH
    NSUB = LCH // P  # 4

    nc = bacc.Bacc("TRN2", target_bir_lowering=False, debug=False)
    x_d = nc.dram_tensor("x", [L, DIM], F32, kind="ExternalInput").ap()
    qkvw_d = nc.dram_tensor("qkv_w", [3 * DIM, DIM], F32, kind="ExternalInput").ap()
    qkvb_d = nc.dram_tensor("qkv_b", [3 * DIM], F32, kind="ExternalInput").ap()
    projw_d = nc.dram_tensor("proj_w", [DIM, DIM], F32, kind="ExternalInput").ap()
    projb_d = nc.dram_tensor("proj_b", [DIM], F32, kind="ExternalInput").ap()
    pm_d = nc.dram_tensor("proj_mat", [M, D], F32, kind="ExternalInput").ap()
    y_d = nc.dram_tensor("y", [L, DIM], F32, kind="ExternalOutput").ap()

    with tile.TileContext(nc) as tc:
        with ExitStack() as ctx:
            _body(ctx, tc, x_d, qkvw_d, qkvb_d, projw_d, projb_d, pm_d, y_d,
                  L, LCH, NCH, NSUB, has_qkv_b, has_proj_b)
    nc.compile()
    return nc


def _dr_gemm(nc, out, whl, xhl, wcols, lt=None, bias=None):
    """Accumulating fp8 DoubleRow hi/lo-compensated GEMM over 768 contraction.

    whl/xhl: [128, KT, 2, *] fp8 with hi at [:,kk,0,:], lo at [:,kk,1,:].
    lt=None : out[wc, l]; stationary = whl cols wcols, moving = xhl  (kT/qT)
    lt given: out[l, wc]; stationary = xhl l-tile cols, moving = whl (v/y)
    """
    c0, c1 = wcols
    n = c1 - c0
    if lt is None:
        for i in range(KT // 2):
            for kk in (2 * i, 2 * i + 1):
                nc.tensor.matmul(
                    out, whl[:, kk, :, c0:c1],
                    xhl[:, kk, 0:1, :].to_broadcast([P, 2, out.shape[-1]]),
                    start=(kk == 0), stop=False, perf_mode=DR,
                )
            nc.tensor.matmul(
                out, whl[:, 2 * i : 2 * i + 2, 0, c0:c1],
                xhl[:, 2 * i : 2 * i + 2, 1, :],
                start=False, stop=(bias is None and i == KT // 2 - 1),
                perf_mode=DR,
            )
    else:
        l0 = lt * P
        for i in range(KT // 2):
            for kk in (2 * i, 2 * i + 1):
                nc.tensor.matmul(
                    out, xhl[:, kk, :, l0 : l0 + P],
                    whl[:, kk, 0:1, c0:c1].to_broadcast([P, 2, n]),
                    start=(kk == 0), stop=False, perf_mode=DR,
                )
            nc.tensor.matmul(
                out, xhl[:, 2 * i : 2 * i + 2, 0, l0 : l0 + P],
                whl[:, 2 * i : 2 * i + 2, 1, c0:c1],
                start=False, stop=(bias is None and i == KT // 2 - 1),
                perf_mode=DR,
            )
    if bias is not None:
        ones_row, brow = bias
        nc.tensor.matmul(out, ones_row, brow, start=False, stop=True)


def _body(ctx, tc, x_d, qkvw_d, qkvb_d, projw_d, projb_d, pm_d, y_d,
          L, LCH, NCH, NSUB, has_qkv_b, has_proj_b):
    nc = tc.nc
    iqkv = 1.0 / (SX * SW)
    iy = 1.0 / (SA * SP)

    persist = ctx.enter_context(tc.tile_pool(name="persist", bufs=1))

    ident16 = persist.tile([P, P], F16, tag="ident16", name="ident16")[:]
    nc.gpsimd.memset(ident16, 0.0)
    nc.gpsimd.affine_select(
        out=ident16, in_=ident16, compare_op=AL.not_equal, fill=1.0,
        base=0, pattern=[[-1, P]], channel_multiplier=1,
    )
    ones1 = persist.tile([1, P], F16, tag="ones1", name="ones1")[:]
    nc.gpsimd.memset(ones1, 1.0)
    epscol = persist.tile([P, 1], F16, tag="epscol", name="epscol")[:]
    nc.gpsimd.memset(epscol, EPS)
    epsb = persist.tile([P, 1], F32, tag="epsb", name="epsb")[:]
    nc.gpsimd.memset(epsb, SKP * EPS)

    whl_qk = persist.tile([P, KT, 2, 2 * DIM], F8, tag="whl_qk", name="whl_qk")[:]
    wvhl = persist.tile([P, KT, 2, DIM], F8, tag="wvhl", name="wvhl")[:]
    wphl = persist.tile([P, KT, 2, DIM], F8, tag="wphl", name="wphl")[:]
    # folded blockdiag pm for the fp8 DoubleRow k-feature GEMM:
    # slice 0 = [RATIO*pmT | 0] (c rows 0:64), slice 1 = [0 | RATIO*pmT]
    pmbd = persist.tile([P, 2, 2 * M], F8, tag="pmbd", name="pmbd")[:]
    pmt2 = persist.tile([P, M], F16, tag="pmt2", name="pmt2")[:]
    qt_sb = persist.tile([P, NPAIR, L], F16, tag="qt_sb", name="qt_sb")[:]
    kvm = persist.tile([P, 4, 6, D + 1], F16, tag="kvm", name="kvm")[:]
    kvmcs = persist.tile([1, 4, 6, D + 1], F16, tag="kvmcs", name="kvmcs")[:]

    if has_qkv_b:
        qkb = persist.tile([P, 2 * KT], F32, tag="qkb", name="qkb")[:]
        nc.sync.dma_start(qkb, qkvb_d.rearrange("(t p) -> p t", p=P)[:, 0 : 2 * KT])
        qkbk = persist.tile([P, KT], F32, tag="qkbk", name="qkbk")[:]
        nc.scalar.activation(qkbk, qkb[:, KT : 2 * KT], AF.Copy, scale=SKT)
        vbf = persist.tile([1, DIM], F32, tag="vbf", name="vbf")[:]
        nc.sync.dma_start(vbf, qkvb_d[2 * DIM : 3 * DIM].unsqueeze(0))
        vb_row = persist.tile([1, DIM], F16, tag="vb_row", name="vb_row")[:]
        nc.scalar.activation(vb_row, vbf, AF.Copy, scale=SX * SW)
    if has_proj_b:
        pbf = persist.tile([1, DIM], F32, tag="pbf", name="pbf")[:]
        nc.sync.dma_start(pbf, projb_d.unsqueeze(0))
        pb_row = persist.tile([1, DIM], F16, tag="pb_row", name="pb_row")[:]
        nc.scalar.activation(pb_row, pbf, AF.Copy, scale=SA * SP)

    vsb = persist.tile([P, 2, NSUB, H, D + 1], F16, tag="vsb", name="vsb")[:]
    nc.gpsimd.memset(vsb[:, :, :, :, D : D + 1], 1.0)

    # kv accumulator in SBUF fp32 (indexed by ai = h*2+mt)
    kv_acc = persist.tile([P, 2 * H, D + 1], F32, tag="kv_acc", name="kv_acc")[:]

    # pass-2 q-feature tiles, double-buffered by chunk parity
    qp2 = [persist.tile([P, H, 2, LCH], F16, tag=f"qp2_{i}", name=f"qp2_{i}")[:]
           for i in range(2)]

    def emit_qp(ich, heads, psum_pool, tag):
        l0 = ich * LCH
        qp_sb = qp2[ich % 2]
        for h in heads:
            p, h2 = h // 2, h % 2
            for mt in range(2):
                qps = psum_pool.tile([P, LCH], F32, tag=tag, name="qps")[:]
                nc.tensor.matmul(
                    qps,
                    pmt2[h2 * D : (h2 + 1) * D, mt * P : (mt + 1) * P],
                    qt_sb[h2 * D : (h2 + 1) * D, p, l0 : l0 + LCH],
                    start=True, stop=True)
                eng = _QP_ENG[h * 2 + mt]
                if eng == "act":
                    # plain relu; eps restored via rank-1 kvmcs in num
                    nc.scalar.activation(qp_sb[:, h, mt, :], qps, AF.Relu)
                else:
                    nc.vector.tensor_scalar(qp_sb[:, h, mt, :], qps,
                                            EPS, EPS, AL.add, AL.max)

    if True:
        with tc.tile_pool(name="p1x", bufs=2) as xp, \
             tc.tile_pool(name="p1xhl", bufs=2) as xhlp, \
             tc.tile_pool(name="p1kt", bufs=3) as ktp, \
             tc.tile_pool(name="p1kp", bufs=8) as kpp, \
             tc.tile_pool(name="wprep", bufs=3) as wpool, \
             tc.tile_pool(name="p1work", bufs=8, space="PSUM") as work:

            xnats = {}
            xhls = {}

            def dma_x(ich):
                l0 = ich * LCH
                xnat = xp.tile([P, NSUB, DIM], F16, tag="xnat", name="xnat")[:]
                nc.gpsimd.dma_start(
                    xnat,
                    x_d[l0 : l0 + LCH, :].rearrange("(s p) k -> p s k", p=P))
                xnats[ich] = xnat

            xhl_cur = {}

            def transp_x(ich, kks=range(KT)):
                if ich in xhl_cur:
                    xnat, xhl = xhl_cur[ich]
                else:
                    xnat = xnats.pop(ich)
                    xhl = xhlp.tile([P, KT, 2, LCH], F8, tag="xhl",
                                    name="xhl")[:]
                    xhl_cur[ich] = (xnat, xhl)
                for kk in kks:
                    tp = work.tile([P, 512], F16, tag="work", name="ttp")[:]
                    for s in range(NSUB):
                        nc.tensor.transpose(
                            tp[:, s * P : (s + 1) * P],
                            xnat[:, s, kk * P : (kk + 1) * P], ident16)
                    nc.scalar.activation(xhl[:, kk, 0, :], tp, AF.Copy,
                                         scale=SX)
                    nc.vector.scalar_tensor_tensor(
                        xhl[:, kk, 1, :], tp, SX, xhl[:, kk, 0, :],
                        AL.mult, AL.subtract)
                xhls[ich] = xhl

            def emit_v(ich, subs):
                vs = ich % 2
                xhl = xhls[ich]
                for s in subs:
                    for ci in range(2):
                        c0 = ci * 384
                        vps = work.tile([P, 512], F32, tag="work", name="vps")[:]
                        bias = None
                        if has_qkv_b:
                            bias = (ones1, vb_row[:, c0 : c0 + 384])
                        _dr_gemm(nc, vps[:, 0:384], wvhl, xhl, (c0, c0 + 384),
                                 lt=s, bias=bias)
                        nc.scalar.activation(
                            vsb[:, vs, s, 6 * ci : 6 * ci + 6, 0:D],
                            vps[:, 0:384].rearrange("p (h d) -> p h d", d=D),
                            AF.Copy, scale=iqkv)

            def emit_kT(ich, p):
                ktps = work.tile([P, 512], F32, tag="work", name="ktps")[:]
                _dr_gemm(nc, ktps, whl_qk, xhls[ich],
                         (DIM + p * P, DIM + (p + 1) * P))
                # fold [128,512] -> [64, 2, 512] fp8 (scaled) for DoubleRow
                kt = ktp.tile([P, 2, LCH], F8, tag="kt", name="kt")[:]
                for h2 in range(2):
                    if has_qkv_b:
                        nc.scalar.activation(
                            kt[0:D, h2, :], ktps[h2 * D : (h2 + 1) * D, :],
                            AF.Identity,
                            bias=qkbk[h2 * D : (h2 + 1) * D, p : p + 1],
                            scale=SKT * iqkv)
                    else:
                        nc.scalar.activation(
                            kt[0:D, h2, :], ktps[h2 * D : (h2 + 1) * D, :],
                            AF.Copy, scale=SKT * iqkv)
                return kt

            def emit_kp(p, kt):
                kps = []
                for lt in range(NSUB):
                    kpps = work.tile([P, 512], F32, tag="work", name="kpps")[:]
                    nc.tensor.matmul(kpps,
                                     kt[0:D, :, lt * P : (lt + 1) * P],
                                     pmbd[0:D], start=True, stop=True,
                                     perf_mode=DR)
                    kp = kpp.tile([P, 2 * M], F16, tag="kp", name="kp")[:]
                    # k-feature path is scaled by SKP; it cancels in num/den.
                    # ACT-assigned tiles use relu(z+eps) ~ relu(z)+eps
                    # (differs by <= eps only for z in (-eps, 0)); exact
                    # max(z+eps, eps) on DVE for the rest
                    if lt == 2:
                        nc.scalar.activation(kp, kpps, AF.Relu, bias=epsb)
                    else:
                        nc.vector.tensor_scalar(kp, kpps, SKP * EPS, SKP * EPS,
                                                AL.add, AL.max)
                    kps.append(kp)
                return kps

            def emit_qT(ich, p):
                l0 = ich * LCH
                qtps = work.tile([P, 512], F32, tag="work", name="qtps")[:]
                _dr_gemm(nc, qtps, whl_qk, xhls[ich], (p * P, (p + 1) * P))
                if has_qkv_b:
                    nc.scalar.activation(qt_sb[:, p, l0 : l0 + LCH], qtps,
                                         AF.Identity,
                                         bias=qkb[:, p : p + 1], scale=iqkv)
                else:
                    nc.scalar.activation(qt_sb[:, p, l0 : l0 + LCH], qtps,
                                         AF.Copy, scale=iqkv)

            def emit_kv(ich, p, kps):
                vs = ich % 2
                kvp = work.tile([P, 4, D + 1], F32, tag="work", name="kvp")[:]
                for lt in range(NSUB):
                    kp = kps[lt]
                    for h2 in range(2):
                        h = 2 * p + h2
                        for mt in range(2):
                            j = h2 * 2 + mt
                            nc.tensor.matmul(
                                kvp[:, j, :],
                                kp[:, j * P : (j + 1) * P],
                                vsb[:, vs, lt, h, :],
                                start=(lt == 0 and j == 0),
                                stop=(lt == NSUB - 1 and j == 3),
                            )
                nc.vector.tensor_tensor(
                    kv_acc[:, 4 * p : 4 * p + 4, :], kvp,
                    kv_acc[:, 4 * p : 4 * p + 4, :], AL.add)

            # ---- prep: weight DMA blocks interleaved with chunk-0/1 work ----
            def prep_w_blocks(src, nrows, dst, dst_off, scale):
                blocks = []
                nt = nrows // P
                c0 = 0
                while c0 < nt:
                    bs = min(3, nt - c0)
                    st = {}

                    def bdma(c0=c0, bs=bs, st=st):
                        wnat = wpool.tile([P, 3, DIM], F16, tag="wnat",
                                          name="wnat")[:]
                        nc.gpsimd.dma_start(
                            wnat[:, 0:bs, :],
                            src[c0 * P : (c0 + bs) * P, :].rearrange(
                                "(s p) k -> p s k", p=P))
                        st["wnat"] = wnat

                    def bcomp(c0=c0, bs=bs, st=st):
                        wnat = st["wnat"]
                        for kk in range(KT):
                            tp = work.tile([P, 512], F16, tag="work",
                                           name="ptp")[:]
                            for j in range(bs):
                                nc.tensor.transpose(
                                    tp[:, j * P : (j + 1) * P],
                                    wnat[:, j, kk * P : (kk + 1) * P], ident16)
                            hi = dst[:, kk, 0,
                                     dst_off + c0 * P : dst_off + (c0 + bs) * P]
                            nc.scalar.activation(hi, tp[:, 0 : bs * P], AF.Copy,
                                                 scale=scale)
                            nc.vector.scalar_tensor_tensor(
                                dst[:, kk, 1,
                                    dst_off + c0 * P : dst_off + (c0 + bs) * P],
                                tp[:, 0 : bs * P], scale, hi,
                                AL.mult, AL.subtract)

                    blocks.append((bdma, bcomp))
                    c0 += bs
                return blocks

            pm_st = {}

            def prep_pm_dma():
                pmn = wpool.tile([P, 2, D], F16, tag="pmn", name="pmn")[:]
                nc.gpsimd.dma_start(pmn, pm_d.rearrange("(s p) d -> p s d", p=P))
                pm_st["pmn"] = pmn

            def prep_pm():
                pmn = pm_st["pmn"]
                tp = work.tile([P, 512], F16, tag="work", name="ptp")[:]
                for s in range(2):
                    nc.tensor.transpose(tp[0:D, s * P : (s + 1) * P],
                                        pmn[:, s, :], ident16)
                nc.gpsimd.memset(pmbd, 0.0)
                nc.scalar.activation(pmbd[0:D, 0, 0:M], tp[0:D, 0:M], AF.Copy,
                                     scale=SPM * RATIO)
                nc.scalar.activation(pmbd[0:D, 1, M : 2 * M], tp[0:D, 0:M],
                                     AF.Copy, scale=SPM * RATIO)
                nc.scalar.activation(pmt2[0:D, :], tp[0:D, 0:M], AF.Copy,
                                     scale=RATIO)
                nc.scalar.activation(pmt2[D:P, :], tp[0:D, 0:M], AF.Copy,
                                     scale=RATIO)

            dma_x(0)
            wv = prep_w_blocks(qkvw_d[2 * DIM : 3 * DIM, :], DIM, wvhl, 0, SW)
            wqk_k = prep_w_blocks(qkvw_d[DIM : 2 * DIM, :], DIM, whl_qk,
                                  DIM, SW)
            wqk_q = prep_w_blocks(qkvw_d[0:DIM, :], DIM, whl_qk, 0, SW)
            wp = prep_w_blocks(projw_d, DIM, wphl, 0, SP)

            nc.gpsimd.memset(kv_acc, 0.0)
            blocks = wv + wqk_k + wqk_q + wp
            bst = {"dma": 0, "comp": 0}

            def bdma_next():
                if bst["dma"] < len(blocks):
                    blocks[bst["dma"]][0]()
                    bst["dma"] += 1

            def bcomp_next():
                if bst["comp"] < len(blocks):
                    blocks[bst["comp"]][1]()
                    bst["comp"] += 1
                    bdma_next()

            nop = lambda: None
            # phase A: wv + k-part of wqk; q-part and proj stream into chunk 0
            nA = len(wv) + len(wqk_k)
            fillers = {
                0: [lambda: transp_x(0, range(0, 4)),
                    (lambda: dma_x(1)) if 1 < NCH else nop],
                1: [lambda: transp_x(0, range(4, KT)), prep_pm],
                2: [lambda: emit_v(0, (0,)), lambda: emit_v(0, (1,)),
                    (lambda: transp_x(1, range(0, 4))) if 1 < NCH else nop],
                3: [lambda: emit_v(0, (2,)), lambda: emit_v(0, (3,)),
                    (lambda: transp_x(1, range(4, KT))) if 1 < NCH else nop,
                    (lambda: dma_x(2)) if 2 < NCH else nop],
            }
            bdma_next()
            prep_pm_dma()
            bdma_next()
            # PE p-state warmup: burn the cold DMA-wait ramping the clock
            wu = work.tile([P, 512], F16, tag="work", name="wu")[:]
            for _ in range(12):
                for s in range(4):
                    nc.tensor.transpose(wu[:, s * P : (s + 1) * P], ident16,
                                        ident16)
            for i in range(nA):
                for f in fillers.get(i, []):
                    f()
                bcomp_next()

            # ---- pass 1 main loop ----
            for ich in range(NCH):
                first = ich == 0
                pend_kt = None
                pend = None
                for p in range(NPAIR):
                    kt = emit_kT(ich, p)
                    if first:
                        # stream remaining weight blocks (q-part + proj)
                        bcomp_next()
                        if p >= 3:
                            emit_qT(0, p - 3)
                    else:
                        emit_qT(ich, p)
                    if pend is not None:
                        emit_kv(ich, pend[0], pend[1])
                        if ich == NCH - 1:
                            pp = pend[0]
                            nc.scalar.activation(
                                kvm.rearrange("p b j c -> p (b j) c")[
                                    :, 4 * pp : 4 * pp + 4, :],
                                kv_acc[:, 4 * pp : 4 * pp + 4, :],
                                AF.Copy, scale=1.0 / SKP)
                        pend = None
                    if pend_kt is not None:
                        pend = (pend_kt[0], emit_kp(pend_kt[0], pend_kt[1]))
                    pend_kt = (p, kt)
                    if ich == NCH - 1 and not first:
                        # chunk-0 q features computed here so pass 2 starts hot
                        emit_qp(0, (2 * p, 2 * p + 1), work, "work")
                    if p == 0 and 1 <= ich and ich + 1 < NCH:
                        transp_x(ich + 1, range(0, 4))
                    if p == 2 and 1 <= ich and ich + 1 < NCH:
                        transp_x(ich + 1, range(4, KT))
                    if p == 3 and 1 <= ich and ich + 2 < NCH:
                        dma_x(ich + 2)
                    if p >= 3 and ich + 1 < NCH:
                        emit_v(ich + 1, (p - 3,))
                if pend is not None:
                    emit_kv(ich, pend[0], pend[1])
                    if ich == NCH - 1:
                        pp = pend[0]
                        nc.scalar.activation(
                            kvm.rearrange("p b j c -> p (b j) c")[
                                :, 4 * pp : 4 * pp + 4, :],
                            kv_acc[:, 4 * pp : 4 * pp + 4, :],
                            AF.Copy, scale=1.0 / SKP)
                pend = (pend_kt[0], emit_kp(pend_kt[0], pend_kt[1]))
                if ich + 1 < NCH:
                    emit_v(ich + 1, (3,))
                emit_kv(ich, pend[0], pend[1])
                if ich == NCH - 1:
                    nc.scalar.activation(
                        kvm.rearrange("p b j c -> p (b j) c")[:, 20:24, :],
                        kv_acc[:, 20:24, :], AF.Copy, scale=1.0 / SKP)
                if first:
                    for p3 in range(3, NPAIR):
                        emit_qT(0, p3)
                    if NCH == 1:
                        emit_qp(0, range(H), work, "work")
                xhls.pop(ich)

    with tc.tile_pool(name="csps", bufs=1, space="PSUM") as cspool:
        css = []
        for b in range(4):
            cs = cspool.tile([1, 6, D + 1], F32, tag=f"cs{b}", name="cs")[:]
            for j in range(6):
                nc.tensor.matmul(cs[:, j, :], epscol, kvm[:, b, j, :],
                                 start=(j == 0), stop=(j == 5))
            css.append(cs)
        for b in range(4):
            nc.scalar.copy(kvmcs[:, b], css[b])

    # ---- pass 2 ----
    with tc.tile_pool(name="p2attn", bufs=2) as atp_sb, \
         tc.tile_pool(name="p2rd", bufs=2) as rdp, \
         tc.tile_pool(name="p2ahl", bufs=2) as ahlp, \
         tc.tile_pool(name="p2y", bufs=2) as yp, \
         tc.tile_pool(name="ps2qp", bufs=3, space="PSUM") as qppsum, \
         tc.tile_pool(name="ps2nm", bufs=2, space="PSUM") as numpsum, \
         tc.tile_pool(name="ps2at", bufs=1, space="PSUM") as atpsum, \
         tc.tile_pool(name="ps2y", bufs=1, space="PSUM") as ypsum:

        def emit_y(ich, ahl, lt):
            l0 = ich * LCH
            yps = ypsum.tile([P, DIM], F32, tag="yps", name="yps")[:]
            for c0, c1 in ((0, 512), (512, DIM)):
                b = (ones1, pb_row[:, c0:c1]) if has_proj_b else None
                _dr_gemm(nc, yps[:, c0:c1], wphl, ahl, (c0, c1), lt=lt, bias=b)
            ysb = yp.tile([P, DIM], F32, tag="ysb", name="ysb")[:]
            nc.scalar.activation(ysb, yps, AF.Copy, scale=iy)
            nc.sync.dma_start(y_d[l0 + lt * P : l0 + (lt + 1) * P, :], ysb)

        pend_y = []
        for ich in range(NCH):
            qp_sb = qp2[ich % 2]
            ahl = ahlp.tile([P, KT, 2, LCH], F8, tag="ahl", name="ahl")[:]
            for lt in range(NSUB):
                nmps = [numpsum.tile([P, 6, D + 1], F32, tag="nm", name="nmps")[:]
                        for _ in range(2)]
                for h in range(H):
                    g = h // 6
                    for mt in range(2):
                        ai = h * 2 + mt
                        if _QP_ENG[ai] == "act" and h % 6 != 0:
                            nc.tensor.matmul(
                                nmps[g][:, h % 6, :], ones1,
                                kvmcs[0:1, ai // 6, ai % 6, :],
                                start=False, stop=False)
                    for mt in range(2):
                        ai = h * 2 + mt
                        nc.tensor.matmul(
                            nmps[g][:, h % 6, :],
                            qp_sb[:, h, mt, lt * P : (lt + 1) * P],
                            kvm[:, ai // 6, ai % 6, :],
                            start=(mt == 0 and h % 6 == 0),
                            stop=(mt == 1 and h % 6 == 5))
                rd = rdp.tile([P, H], F32, tag="rd", name="rd")[:]
                attn = atp_sb.tile([P, H, D], F16, tag="attn", name="attn")[:]
                for g in range(2):
                    nc.vector.reciprocal(rd[:, g * 6 : (g + 1) * 6],
                                         nmps[g][:, :, D])
                    nc.vector.tensor_tensor(
                        attn[:, g * 6 : (g + 1) * 6, :],
                        nmps[g][:, :, 0:D],
                        rd[:, g * 6 : (g + 1) * 6, None].to_broadcast([P, 6, D]),
                        AL.mult)
                if ich + 1 < NCH:
                    emit_qp(ich + 1, range(3 * lt, 3 * lt + 3), qppsum, "qps")
                if len(pend_y) >= (2 if ich + 1 < NCH else 1):
                    emit_y(*pend_y.pop(0))
                atps = atpsum.tile([P, DIM], F16, tag="at", name="atps")[:]
                for kk in range(KT):
                    nc.tensor.transpose(atps[:, kk * P : (kk + 1) * P],
                                        attn[:, 2 * kk : 2 * kk + 2, :], ident16)
                a3 = atps.rearrange("p (k l) -> p k l", k=KT)
                nc.scalar.activation(ahl[:, :, 0, lt * P : (lt + 1) * P], a3,
                                     AF.Copy, scale=SA)
                nc.vector.scalar_tensor_tensor(
                    ahl[:, :, 1, lt * P : (lt + 1) * P], a3, SA,
                    ahl[:, :, 0, lt * P : (lt + 1) * P], AL.mult, AL.subtract)
                pend_y.append((ich, ahl, lt))
        for args in pend_y:
            emit_y(*args)


_CACHE = {}


def _get_nc(L=4096, hqb=False, hpb=False):
    key = ("nc", L, hqb, hpb)
    if key not in _CACHE:
        _CACHE[key] = build(L, hqb, hpb)
    return _CACHE[key]


last_exec_time_ns = None
last_profile = None


def kernel(x, qkv_w, qkv_b, proj_w, proj_b, proj_mat):
    global last_exec_time_ns, last_profile
    from concourse.bass_utils import run_bass_kernel_spmd

    x = np.asarray(x, np.float32)
    B, L, _ = x.shape
    hqb = bool(np.any(np.asarray(qkv_b)))
    hpb = bool(np.any(np.asarray(proj_b)))
    nc = _get_nc(L, hqb, hpb)
    base = {
        "qkv_w": np.ascontiguousarray(np.asarray(qkv_w, np.float32)),
        "qkv_b": np.ascontiguousarray(np.asarray(qkv_b, np.float32)),
        "proj_w": np.ascontiguousarray(np.asarray(proj_w, np.float32)),
        "proj_b": np.ascontiguousarray(np.asarray(proj_b, np.float32)),
        "proj_mat": np.ascontiguousarray(np.asarray(proj_mat, np.float32)),
    }
    in_maps = [dict(base, x=np.ascontiguousarray(x[b])) for b in range(B)]
    trace = bool(int(os.environ.get("KERNEL_TRACE", "0")))
    res = run_bass_kernel_spmd(nc, in_maps, core_ids=list(range(B)), trace=trace)
    last_exec_time_ns = res.exec_time_ns
    last_profile = res.profile_json
    return np.stack([res.results[b]["y"] for b in range(B)], axis=0)


def _ref_np(x, qkv_w, qkv_b, proj_w, proj_b, proj_mat):
    Ls = x.shape[0]
    qkv = x @ qkv_w.T + qkv_b
    qkv = qkv.reshape(Ls, 3, H, D)
    q, k, v = qkv[:, 0], qkv[:, 1], qkv[:, 2]
    qp = np.maximum(RATIO * np.einsum("lhd,md->lhm", q, proj_mat), 0) + EPS
    kp = np.maximum(RATIO * np.einsum("lhd,md->lhm", k, proj_mat), 0) + EPS
    kv = np.einsum("lhm,lhd->hmd", kp, v)
    ks = kp.sum(axis=0)
    num = np.einsum("lhm,hmd->lhd", qp, kv)
    den = np.einsum("lhm,hm->lh", qp, ks)
    out = (num / den[..., None]).reshape(Ls, DIM)
    return out @ proj_w.T + proj_b


if __name__ == "__main__":
    from concourse.bass_interp import CoreSim

    Ls = int(os.environ.get("SIM_L", "512"))
    use_bias = bool(int(os.environ.get("SIM_BIAS", "1")))
    rng = np.random.default_rng(0)
    x = rng.standard_normal((Ls, DIM), dtype=np.float32)
    qkv_w = (rng.standard_normal((3 * DIM, DIM), dtype=np.float32) * DIM**-0.5)
    qkv_b = (rng.standard_normal(3 * DIM, dtype=np.float32) * 0.1
             if use_bias else np.zeros(3 * DIM, np.float32))
    proj_w = (rng.standard_normal((DIM, DIM), dtype=np.float32) * DIM**-0.5)
    proj_b = (rng.standard_normal(DIM, dtype=np.float32) * 0.1
              if use_bias else np.zeros(DIM, np.float32))
    proj_mat = rng.standard_normal((M, D), dtype=np.float32)

    print(f"building L={Ls} bias={use_bias} ...")
    nc = build(Ls, use_bias, use_bias)
    print("simulating ...")
    sim = CoreSim(nc)
    for name, arr in [("x", x), ("qkv_w", qkv_w), ("qkv_b", qkv_b),
                      ("proj_w", proj_w), ("proj_b", proj_b),
                      ("proj_mat", proj_mat)]:
        sim.tensor(name)[:] = arr
    sim.simulate(check_with_hw=False)
    got = np.array(sim.tensor("y"))
    want = _ref_np(x, qkv_w, qkv_b, proj_w, proj_b, proj_mat)
    err = np.abs(got - want)
    rel = np.linalg.norm(got - want) / np.linalg.norm(want)
    print("max abs err:", err.max(), " rel fro err:", rel)
    assert rel < 2e-2, "sim mismatch"
    print("SIM OK")


# revision 19
# speedup vs baseline: 1.0288x; 1.0005x over previous
"""FAVOR+ (Performer) non-causal linear attention on 8 Trainium2 NeuronCores.

Sharding: data-parallel over batch B=8 -> one batch element per core.

Per-core pipeline (L=4096, DIM=768, H=12, D=64, M=256):
  prep : cast-DMA weights to fp16, PE-transpose to feature-major, split into
         fp8e4m3 hi/lo pairs (scaled) for DoubleRow matmuls; DMA blocks
         interleaved with chunk-0/1 transposes and v so PE never starves
  pass1: per 512-row chunk: cast-DMA x to fp16; PE-transpose -> xT hi/lo fp8;
         kT/qT/v via fp8 DoubleRow hi/lo-compensated GEMMs (4.5 cyc per
         128x512 output tile instead of 6); k features fp16 with blockdiag pm
         (2 heads per matmul); kv accumulated m-major [m,65] into 4 persistent
         PSUM banks across all chunks (ones-augmented v gives k_sum for
         free); qT staged in SBUF fp16 (no DRAM round trip).  Emission is
         software-pipelined: transposes/v run 2 chunks ahead; kv trails one
         pair behind its kp conversion.
  mid  : kv PSUM -> fp16 SBUF (already m-major); eps*colsum(kv) rows for the
         ACT-assigned q-feature heads
  pass2: q features m-major fp16 (computed one chunk ahead, spread across the
         l-tile loop; relu+eps on DVE/Pool, plain relu on ACT with the eps
         restored by a rank-1 matmul into num); num L-major [l,65] (den =
         col 64); reciprocal + broadcast multiply on DVE; attn PE-transposed
         to feature-major, split fp8 hi/lo; y via DoubleRow GEMM -> DMA out
"""

import math
import os
import sys
from contextlib import ExitStack

import numpy as np

for _p in ("/opt/trn_rl_repo",):
    if _p not in sys.path and os.path.isdir(_p):
        sys.path.insert(0, _p)

import concourse.bass as bass  # noqa: E402
import concourse.mybir as mybir  # noqa: E402
import concourse.tile as tile  # noqa: E402
from concourse import bacc  # noqa: E402

P = 128
DIM = 768
H = 12
D = 64
M = 256
KT = DIM // P   # 6 contraction c-tiles
NPAIR = H // 2  # 6 head pairs
EPS = 1e-3
RATIO = 1.0 / math.sqrt(float(M))

SX = 16.0    # x ~ N(0,1)
SW = 32.0    # qkv_w ~ N(0, 1/768)
SA = 64.0    # attn ~ 0.1
SP = 32.0    # proj_w ~ N(0, 1/768)
SKT = 16.0   # kT ~ N(0,1) -> fp8 for the k-feature GEMM
SPM = 32.0   # RATIO*pm ~ N(0, 1/256) -> fp8
SKP = SKT * SPM  # k-feature path runs in this scaled domain until kvm

F32 = mybir.dt.float32
F16 = mybir.dt.float16
F8 = mybir.dt.float8e4
AL = mybir.AluOpType
AF = mybir.ActivationFunctionType
DR = mybir.MatmulPerfMode.DoubleRow

# pass-2 q-feature conversion engine per (head, mtile) slot ai=h*2+mt.
# Slots of the group-starting heads (ai 0,1,12,13) stay off ACT so each num
# PSUM group's first instruction is its start matmul.
_QP_ENG = {}
_c = 0
for _ai in range(2 * H):
    if _ai in (0, 1, 12, 13):
        _QP_ENG[_ai] = "dve"
    else:
        _QP_ENG[_ai] = ("dve", "act", "act")[_c % 3]
        _c += 1


def build(L=4096, has_qkv_b=False, has_proj_b=False):
    LCH = 512
    NCH = L // LCH
    NSUB = LCH // P  # 4

    nc = bacc.Bacc("TRN2", target_bir_lowering=False, debug=False)
    x_d = nc.dram_tensor("x", [L, DIM], F32, kind="ExternalInput").ap()
    qkvw_d = nc.dram_tensor("qkv_w", [3 * DIM, DIM], F32, kind="ExternalInput").ap()
    qkvb_d = nc.dram_tensor("qkv_b", [3 * DIM], F32, kind="ExternalInput").ap()
    projw_d = nc.dram_tensor("proj_w", [DIM, DIM], F32, kind="ExternalInput").ap()
    projb_d = nc.dram_tensor("proj_b", [DIM], F32, kind="ExternalInput").ap()
    pm_d = nc.dram_tensor("proj_mat", [M, D], F32, kind="ExternalInput").ap()
    y_d = nc.dram_tensor("y", [L, DIM], F32, kind="ExternalOutput").ap()

    with tile.TileContext(nc) as tc:
        with ExitStack() as ctx:
            _body(ctx, tc, x_d, qkvw_d, qkvb_d, projw_d, projb_d, pm_d, y_d,
                  L, LCH, NCH, NSUB, has_qkv_b, has_proj_b)
    nc.compile()
    return nc


def _dr_gemm(nc, out, whl, xhl, wcols, lt=None, bias=None):
    """Accumulating fp8 DoubleRow hi/lo-compensated GEMM over 768 contraction.

    whl/xhl: [128, KT, 2, *] fp8 with hi at [:,kk,0,:], lo at [:,kk,1,:].
    lt=None : out[wc, l]; stationary = whl cols wcols, moving = xhl  (kT/qT)
    lt given: out[l, wc]; stationary = xhl l-tile cols, moving = whl (v/y)
    """
    c0, c1 = wcols
    n = c1 - c0
    if lt is None:
        for i in range(KT // 2):
            for kk in (2 * i, 2 * i + 1):
                nc.tensor.matmul(
                    out, whl[:, kk, :, c0:c1],
                    xhl[:, kk, 0:1, :].to_broadcast([P, 2, out.shape[-1]]),
                    start=(kk == 0), stop=False, perf_mode=DR,
                )
            nc.tensor.matmul(
                out, whl[:, 2 * i : 2 * i + 2, 0, c0:c1],
                xhl[:, 2 * i : 2 * i + 2, 1, :],
                start=False, stop=(bias is None and i == KT // 2 - 1),
                perf_mode=DR,
            )
    else:
        l0 = lt * P
        for i in range(KT // 2):
            for kk in (2 * i, 2 * i + 1):
                nc.tensor.matmul(
                    out, xhl[:, kk, :, l0 : l0 + P],
                    whl[:, kk, 0:1, c0:c1].to_broadcast([P, 2, n]),
                    start=(kk == 0), stop=False, perf_mode=DR,
                )
            nc.tensor.matmul(
                out, xhl[:, 2 * i : 2 * i + 2, 0, l0 : l0 + P],
                whl[:, 2 * i : 2 * i + 2, 1, c0:c1],
                start=False, stop=(bias is None and i == KT // 2 - 1),
                perf_mode=DR,
            )
    if bias is not None:
        ones_row, brow = bias
        nc.tensor.matmul(out, ones_row, brow, start=False, stop=True)


def _body(ctx, tc, x_d, qkvw_d, qkvb_d, projw_d, projb_d, pm_d, y_d,
          L, LCH, NCH, NSUB, has_qkv_b, has_proj_b):
    nc = tc.nc
    iqkv = 1.0 / (SX * SW)
    iy = 1.0 / (SA * SP)

    persist = ctx.enter_context(tc.tile_pool(name="persist", bufs=1))

    ident16 = persist.tile([P, P], F16, tag="ident16", name="ident16")[:]
    nc.gpsimd.memset(ident16, 0.0)
    nc.gpsimd.affine_select(
        out=ident16, in_=ident16, compare_op=AL.not_equal, fill=1.0,
        base=0, pattern=[[-1, P]], channel_multiplier=1,
    )
    ones1 = persist.tile([1, P], F16, tag="ones1", name="ones1")[:]
    nc.gpsimd.memset(ones1, 1.0)
    epscol = persist.tile([P, 1], F16, tag="epscol", name="epscol")[:]
    nc.gpsimd.memset(epscol, EPS)
    epsb = persist.tile([P, 1], F32, tag="epsb", name="epsb")[:]
    nc.gpsimd.memset(epsb, SKP * EPS)

    whl_qk = persist.tile([P, KT, 2, 2 * DIM], F8, tag="whl_qk", name="whl_qk")[:]
    wvhl = persist.tile([P, KT, 2, DIM], F8, tag="wvhl", name="wvhl")[:]
    wphl = persist.tile([P, KT, 2, DIM], F8, tag="wphl", name="wphl")[:]
    # folded blockdiag pm for the fp8 DoubleRow k-feature GEMM:
    # slice 0 = [RATIO*pmT | 0] (c rows 0:64), slice 1 = [0 | RATIO*pmT]
    pmbd = persist.tile([P, 2, 2 * M], F8, tag="pmbd", name="pmbd")[:]
    pmt2 = persist.tile([P, M], F16, tag="pmt2", name="pmt2")[:]
    qt_sb = persist.tile([P, NPAIR, L], F16, tag="qt_sb", name="qt_sb")[:]
    kvm = persist.tile([P, 4, 6, D + 1], F16, tag="kvm", name="kvm")[:]
    kvmcs = persist.tile([1, 4, 6, D + 1], F16, tag="kvmcs", name="kvmcs")[:]

    if has_qkv_b:
        qkb = persist.tile([P, 2 * KT], F32, tag="qkb", name="qkb")[:]
        nc.sync.dma_start(qkb, qkvb_d.rearrange("(t p) -> p t", p=P)[:, 0 : 2 * KT])
        qkbk = persist.tile([P, KT], F32, tag="qkbk", name="qkbk")[:]
        nc.scalar.activation(qkbk, qkb[:, KT : 2 * KT], AF.Copy, scale=SKT)
        vbf = persist.tile([1, DIM], F32, tag="vbf", name="vbf")[:]
        nc.sync.dma_start(vbf, qkvb_d[2 * DIM : 3 * DIM].unsqueeze(0))
        vb_row = persist.tile([1, DIM], F16, tag="vb_row", name="vb_row")[:]
        nc.scalar.activation(vb_row, vbf, AF.Copy, scale=SX * SW)
    if has_proj_b:
        pbf = persist.tile([1, DIM], F32, tag="pbf", name="pbf")[:]
        nc.sync.dma_start(pbf, projb_d.unsqueeze(0))
        pb_row = persist.tile([1, DIM], F16, tag="pb_row", name="pb_row")[:]
        nc.scalar.activation(pb_row, pbf, AF.Copy, scale=SA * SP)

    vsb = persist.tile([P, 2, NSUB, H, D + 1], F16, tag="vsb", name="vsb")[:]
    nc.gpsimd.memset(vsb[:, :, :, :, D : D + 1], 1.0)

    # kv accumulator in SBUF fp32 (indexed by ai = h*2+mt)
    kv_acc = persist.tile([P, 2 * H, D + 1], F32, tag="kv_acc", name="kv_acc")[:]

    # pass-2 q-feature tiles, double-buffered by chunk parity
    qp2 = [persist.tile([P, H, 2, LCH], F16, tag=f"qp2_{i}", name=f"qp2_{i}")[:]
           for i in range(2)]

    def emit_qp(ich, heads, psum_pool, tag):
        l0 = ich * LCH
        qp_sb = qp2[ich % 2]
        for h in heads:
            p, h2 = h // 2, h % 2
            for mt in range(2):
                qps = psum_pool.tile([P, LCH], F32, tag=tag, name="qps")[:]
                nc.tensor.matmul(
                    qps,
                    pmt2[h2 * D : (h2 + 1) * D, mt * P : (mt + 1) * P],
                    qt_sb[h2 * D : (h2 + 1) * D, p, l0 : l0 + LCH],
                    start=True, stop=True)
                eng = _QP_ENG[h * 2 + mt]
                if eng == "act":
                    # plain relu; eps restored via rank-1 kvmcs in num
                    nc.scalar.activation(qp_sb[:, h, mt, :], qps, AF.Relu)
                else:
                    nc.vector.tensor_scalar(qp_sb[:, h, mt, :], qps,
                                            EPS, EPS, AL.add, AL.max)

    if True:
        with tc.tile_pool(name="p1x", bufs=2) as xp, \
             tc.tile_pool(name="p1xhl", bufs=2) as xhlp, \
             tc.tile_pool(name="p1kt", bufs=3) as ktp, \
             tc.tile_pool(name="p1kp", bufs=8) as kpp, \
             tc.tile_pool(name="wprep", bufs=3) as wpool, \
             tc.tile_pool(name="p1work", bufs=8, space="PSUM") as work:

            xnats = {}
            xhls = {}

            def dma_x(ich):
                l0 = ich * LCH
                xnat = xp.tile([P, NSUB, DIM], F16, tag="xnat", name="xnat")[:]
                nc.gpsimd.dma_start(
                    xnat,
                    x_d[l0 : l0 + LCH, :].rearrange("(s p) k -> p s k", p=P))
                xnats[ich] = xnat

            xhl_cur = {}

            def transp_x(ich, kks=range(KT)):
                if ich in xhl_cur:
                    xnat, xhl = xhl_cur[ich]
                else:
                    xnat = xnats.pop(ich)
                    xhl = xhlp.tile([P, KT, 2, LCH], F8, tag="xhl",
                                    name="xhl")[:]
                    xhl_cur[ich] = (xnat, xhl)
                for kk in kks:
                    tp = work.tile([P, 512], F16, tag="work", name="ttp")[:]
                    for s in range(NSUB):
                        nc.tensor.transpose(
                            tp[:, s * P : (s + 1) * P],
                            xnat[:, s, kk * P : (kk + 1) * P], ident16)
                    nc.scalar.activation(xhl[:, kk, 0, :], tp, AF.Copy,
                                         scale=SX)
                    nc.vector.scalar_tensor_tensor(
                        xhl[:, kk, 1, :], tp, SX, xhl[:, kk, 0, :],
                        AL.mult, AL.subtract)
                xhls[ich] = xhl

            def emit_v(ich, subs):
                vs = ich % 2
                xhl = xhls[ich]
                for s in subs:
                    for ci in range(2):
                        c0 = ci * 384
                        vps = work.tile([P, 512], F32, tag="work", name="vps")[:]
                        bias = None
                        if has_qkv_b:
                            bias = (ones1, vb_row[:, c0 : c0 + 384])
                        _dr_gemm(nc, vps[:, 0:384], wvhl, xhl, (c0, c0 + 384),
                                 lt=s, bias=bias)
                        nc.scalar.activation(
                            vsb[:, vs, s, 6 * ci : 6 * ci + 6, 0:D],
                            vps[:, 0:384].rearrange("p (h d) -> p h d", d=D),
                            AF.Copy, scale=iqkv)

            def emit_kT(ich, p):
                ktps = work.tile([P, 512], F32, tag="work", name="ktps")[:]
                _dr_gemm(nc, ktps, whl_qk, xhls[ich],
                         (DIM + p * P, DIM + (p + 1) * P))
                # fold [128,512] -> [64, 2, 512] fp8 (scaled) for DoubleRow
                kt = ktp.tile([P, 2, LCH], F8, tag="kt", name="kt")[:]
                for h2 in range(2):
                    if has_qkv_b:
                        nc.scalar.activation(
                            kt[0:D, h2, :], ktps[h2 * D : (h2 + 1) * D, :],
                            AF.Identity,
                            bias=qkbk[h2 * D : (h2 + 1) * D, p : p + 1],
                            scale=SKT * iqkv)
                    else:
                        nc.scalar.activation(
                            kt[0:D, h2, :], ktps[h2 * D : (h2 + 1) * D, :],
                            AF.Copy, scale=SKT * iqkv)
                return kt

            def emit_kp(p, kt):
                kps = []
                for lt in range(NSUB):
                    kpps = work.tile([P, 512], F32, tag="work", name="kpps")[:]
                    nc.tensor.matmul(kpps,
                                     kt[0:D, :, lt * P : (lt + 1) * P],
                                     pmbd[0:D], start=True, stop=True,
                                     perf_mode=DR)
                    kp = kpp.tile([P, 2 * M], F16, tag="kp", name="kp")[:]
                    # k-feature path is scaled by SKP; it cancels in num/den.
                    # ACT-assigned tiles use relu(z+eps) ~ relu(z)+eps
                    # (differs by <= eps only for z in (-eps, 0)); exact
                    # max(z+eps, eps) on DVE for the rest
                    if lt == (2 if p % 2 == 0 else 1):
                        nc.scalar.activation(kp, kpps, AF.Relu, bias=epsb)
                    else:
                        nc.vector.tensor_scalar(kp, kpps, SKP * EPS, SKP * EPS,
                                                AL.add, AL.max)
                    kps.append(kp)
                return kps

            def emit_qT(ich, p):
                l0 = ich * LCH
                qtps = work.tile([P, 512], F32, tag="work", name="qtps")[:]
                _dr_gemm(nc, qtps, whl_qk, xhls[ich], (p * P, (p + 1) * P))
                if has_qkv_b:
                    nc.scalar.activation(qt_sb[:, p, l0 : l0 + LCH], qtps,
                                         AF.Identity,
                                         bias=qkb[:, p : p + 1], scale=iqkv)
                else:
                    nc.scalar.activation(qt_sb[:, p, l0 : l0 + LCH], qtps,
                                         AF.Copy, scale=iqkv)

            def emit_kv(ich, p, kps):
                vs = ich % 2
                kvp = work.tile([P, 4, D + 1], F32, tag="work", name="kvp")[:]
                for lt in range(NSUB):
                    kp = kps[lt]
                    for h2 in range(2):
                        h = 2 * p + h2
                        for mt in range(2):
                            j = h2 * 2 + mt
                            nc.tensor.matmul(
                                kvp[:, j, :],
                                kp[:, j * P : (j + 1) * P],
                                vsb[:, vs, lt, h, :],
                                start=(lt == 0 and j == 0),
                                stop=(lt == NSUB - 1 and j == 3),
                            )
                nc.vector.tensor_tensor(
                    kv_acc[:, 4 * p : 4 * p + 4, :], kvp,
                    kv_acc[:, 4 * p : 4 * p + 4, :], AL.add)

            # ---- prep: weight DMA blocks interleaved with chunk-0/1 work ----
            def prep_w_blocks(src, nrows, dst, dst_off, scale):
                blocks = []
                nt = nrows // P
                c0 = 0
                while c0 < nt:
                    bs = min(3, nt - c0)
                    st = {}

                    def bdma(c0=c0, bs=bs, st=st):
                        wnat = wpool.tile([P, 3, DIM], F16, tag="wnat",
                                          name="wnat")[:]
                        nc.gpsimd.dma_start(
                            wnat[:, 0:bs, :],
                            src[c0 * P : (c0 + bs) * P, :].rearrange(
                                "(s p) k -> p s k", p=P))
                        st["wnat"] = wnat

                    def bcomp(c0=c0, bs=bs, st=st):
                        wnat = st["wnat"]
                        for kk in range(KT):
                            tp = work.tile([P, 512], F16, tag="work",
                                           name="ptp")[:]
                            for j in range(bs):
                                nc.tensor.transpose(
                                    tp[:, j * P : (j + 1) * P],
                                    wnat[:, j, kk * P : (kk + 1) * P], ident16)
                            hi = dst[:, kk, 0,
                                     dst_off + c0 * P : dst_off + (c0 + bs) * P]
                            nc.scalar.activation(hi, tp[:, 0 : bs * P], AF.Copy,
                                                 scale=scale)
                            nc.vector.scalar_tensor_tensor(
                                dst[:, kk, 1,
                                    dst_off + c0 * P : dst_off + (c0 + bs) * P],
                                tp[:, 0 : bs * P], scale, hi,
                                AL.mult, AL.subtract)

                    blocks.append((bdma, bcomp))
                    c0 += bs
                return blocks

            pm_st = {}

            def prep_pm_dma():
                pmn = wpool.tile([P, 2, D], F16, tag="pmn", name="pmn")[:]
                nc.gpsimd.dma_start(pmn, pm_d.rearrange("(s p) d -> p s d", p=P))
                pm_st["pmn"] = pmn

            def prep_pm():
                pmn = pm_st["pmn"]
                tp = work.tile([P, 512], F16, tag="work", name="ptp")[:]
                for s in range(2):
                    nc.tensor.transpose(tp[0:D, s * P : (s + 1) * P],
                                        pmn[:, s, :], ident16)
                nc.gpsimd.memset(pmbd, 0.0)
                nc.scalar.activation(pmbd[0:D, 0, 0:M], tp[0:D, 0:M], AF.Copy,
                                     scale=SPM * RATIO)
                nc.scalar.activation(pmbd[0:D, 1, M : 2 * M], tp[0:D, 0:M],
                                     AF.Copy, scale=SPM * RATIO)
                nc.scalar.activation(pmt2[0:D, :], tp[0:D, 0:M], AF.Copy,
                                     scale=RATIO)
                nc.scalar.activation(pmt2[D:P, :], tp[0:D, 0:M], AF.Copy,
                                     scale=RATIO)

            dma_x(0)
            wv = prep_w_blocks(qkvw_d[2 * DIM : 3 * DIM, :], DIM, wvhl, 0, SW)
            wqk_k = prep_w_blocks(qkvw_d[DIM : 2 * DIM, :], DIM, whl_qk,
                                  DIM, SW)
            wqk_q = prep_w_blocks(qkvw_d[0:DIM, :], DIM, whl_qk, 0, SW)
            wp = prep_w_blocks(projw_d, DIM, wphl, 0, SP)

            nc.gpsimd.memset(kv_acc, 0.0)
            blocks = wv + wqk_k + wqk_q + wp
            bst = {"dma": 0, "comp": 0}

            def bdma_next():
                if bst["dma"] < len(blocks):
                    blocks[bst["dma"]][0]()
                    bst["dma"] += 1

            def bcomp_next():
                if bst["comp"] < len(blocks):
                    blocks[bst["comp"]][1]()
                    bst["comp"] += 1
                    bdma_next()

            nop = lambda: None
            # phase A: wv + k-part of wqk; q-part and proj stream into chunk 0
            nA = len(wv) + len(wqk_k)
            fillers = {
                0: [lambda: transp_x(0, range(0, 4)),
                    (lambda: dma_x(1)) if 1 < NCH else nop],
                1: [lambda: transp_x(0, range(4, KT)), prep_pm],
                2: [lambda: emit_v(0, (0,)), lambda: emit_v(0, (1,)),
                    (lambda: transp_x(1, range(0, 4))) if 1 < NCH else nop],
                3: [lambda: emit_v(0, (2,)), lambda: emit_v(0, (3,)),
                    (lambda: transp_x(1, range(4, KT))) if 1 < NCH else nop,
                    (lambda: dma_x(2)) if 2 < NCH else nop],
            }
            bdma_next()
            prep_pm_dma()
            bdma_next()
            # PE p-state warmup: burn the cold DMA-wait ramping the clock
            wu = work.tile([P, 512], F16, tag="work", name="wu")[:]
            for _ in range(12):
                for s in range(4):
                    nc.tensor.transpose(wu[:, s * P : (s + 1) * P], ident16,
                                        ident16)
            for i in range(nA):
                for f in fillers.get(i, []):
                    f()
                bcomp_next()

            # ---- pass 1 main loop ----
            for ich in range(NCH):
                first = ich == 0
                pend_kt = None
                pend = None
                for p in range(NPAIR):
                    kt = emit_kT(ich, p)
                    if first:
                        # stream remaining weight blocks (q-part + proj)
                        bcomp_next()
                        if p >= 3:
                            emit_qT(0, p - 3)
                    else:
                        emit_qT(ich, p)
                    if pend is not None:
                        emit_kv(ich, pend[0], pend[1])
                        if ich == NCH - 1:
                            pp = pend[0]
                            nc.scalar.activation(
                                kvm.rearrange("p b j c -> p (b j) c")[
                                    :, 4 * pp : 4 * pp + 4, :],
                                kv_acc[:, 4 * pp : 4 * pp + 4, :],
                                AF.Copy, scale=1.0 / SKP)
                        pend = None
                    if pend_kt is not None:
                        pend = (pend_kt[0], emit_kp(pend_kt[0], pend_kt[1]))
                    pend_kt = (p, kt)
                    if ich == NCH - 1 and not first:
                        # chunk-0 q features computed here so pass 2 starts hot
                        emit_qp(0, (2 * p, 2 * p + 1), work, "work")
                    if p == 0 and 1 <= ich and ich + 1 < NCH:
                        transp_x(ich + 1, range(0, 4))
                    if p == 2 and 1 <= ich and ich + 1 < NCH:
                        transp_x(ich + 1, range(4, KT))
                    if p == 3 and 1 <= ich and ich + 2 < NCH:
                        dma_x(ich + 2)
                    if p >= 3 and ich + 1 < NCH:
                        emit_v(ich + 1, (p - 3,))
                if pend is not None:
                    emit_kv(ich, pend[0], pend[1])
                    if ich == NCH - 1:
                        pp = pend[0]
                        nc.scalar.activation(
                            kvm.rearrange("p b j c -> p (b j) c")[
                                :, 4 * pp : 4 * pp + 4, :],
                            kv_acc[:, 4 * pp : 4 * pp + 4, :],
                            AF.Copy, scale=1.0 / SKP)
                pend = (pend_kt[0], emit_kp(pend_kt[0], pend_kt[1]))
                if ich + 1 < NCH:
                    emit_v(ich + 1, (3,))
                emit_kv(ich, pend[0], pend[1])
                if ich == NCH - 1:
                    nc.scalar.activation(
                        kvm.rearrange("p b j c -> p (b j) c")[:, 20:24, :],
                        kv_acc[:, 20:24, :], AF.Copy, scale=1.0 / SKP)
                if first:
                    for p3 in range(3, NPAIR):
                        emit_qT(0, p3)
                    if NCH == 1:
                        emit_qp(0, range(H), work, "work")
                xhls.pop(ich)

    with tc.tile_pool(name="csps", bufs=1, space="PSUM") as cspool:
        css = []
        for b in range(4):
            cs = cspool.tile([1, 6, D + 1], F32, tag=f"cs{b}", name="cs")[:]
            for j in range(6):
                nc.tensor.matmul(cs[:, j, :], epscol, kvm[:, b, j, :],
                                 start=(j == 0), stop=(j == 5))
            css.append(cs)
        for b in range(4):
            nc.scalar.copy(kvmcs[:, b], css[b])

    # ---- pass 2 ----
    with tc.tile_pool(name="p2attn", bufs=2) as atp_sb, \
         tc.tile_pool(name="p2rd", bufs=2) as rdp, \
         tc.tile_pool(name="p2ahl", bufs=2) as ahlp, \
         tc.tile_pool(name="p2y", bufs=2) as yp, \
         tc.tile_pool(name="ps2qp", bufs=3, space="PSUM") as qppsum, \
         tc.tile_pool(name="ps2nm", bufs=2, space="PSUM") as numpsum, \
         tc.tile_pool(name="ps2at", bufs=1, space="PSUM") as atpsum, \
         tc.tile_pool(name="ps2y", bufs=1, space="PSUM") as ypsum:

        def emit_y(ich, ahl, lt):
            l0 = ich * LCH
            yps = ypsum.tile([P, DIM], F32, tag="yps", name="yps")[:]
            for c0, c1 in ((0, 512), (512, DIM)):
                b = (ones1, pb_row[:, c0:c1]) if has_proj_b else None
                _dr_gemm(nc, yps[:, c0:c1], wphl, ahl, (c0, c1), lt=lt, bias=b)
            ysb = yp.tile([P, DIM], F32, tag="ysb", name="ysb")[:]
            nc.scalar.activation(ysb, yps, AF.Copy, scale=iy)
            nc.sync.dma_start(y_d[l0 + lt * P : l0 + (lt + 1) * P, :], ysb)

        pend_y = []
        for ich in range(NCH):
            qp_sb = qp2[ich % 2]
            ahl = ahlp.tile([P, KT, 2, LCH], F8, tag="ahl", name="ahl")[:]
            for lt in range(NSUB):
                nmps = [numpsum.tile([P, 6, D + 1], F32, tag="nm", name="nmps")[:]
                        for _ in range(2)]
                for h in range(H):
                    g = h // 6
                    for mt in range(2):
                        ai = h * 2 + mt
                        if _QP_ENG[ai] == "act" and h % 6 != 0:
                            nc.tensor.matmul(
                                nmps[g][:, h % 6, :], ones1,
                                kvmcs[0:1, ai // 6, ai % 6, :],
                                start=False, stop=False)
                    for mt in range(2):
                        ai = h * 2 + mt
                        nc.tensor.matmul(
                            nmps[g][:, h % 6, :],
                            qp_sb[:, h, mt, lt * P : (lt + 1) * P],
                            kvm[:, ai // 6, ai % 6, :],
                            start=(mt == 0 and h % 6 == 0),
                            stop=(mt == 1 and h % 6 == 5))
                rd = rdp.tile([P, H], F32, tag="rd", name="rd")[:]
                attn = atp_sb.tile([P, H, D], F16, tag="attn", name="attn")[:]
                for g in range(2):
                    nc.vector.reciprocal(rd[:, g * 6 : (g + 1) * 6],
                                         nmps[g][:, :, D])
                    nc.vector.tensor_tensor(
                        attn[:, g * 6 : (g + 1) * 6, :],
                        nmps[g][:, :, 0:D],
                        rd[:, g * 6 : (g + 1) * 6, None].to_broadcast([P, 6, D]),
                        AL.mult)
                if ich + 1 < NCH:
                    emit_qp(ich + 1, range(3 * lt, 3 * lt + 3), qppsum, "qps")
                if len(pend_y) >= (2 if ich + 1 < NCH else 1):
                    emit_y(*pend_y.pop(0))
                atps = atpsum.tile([P, DIM], F16, tag="at", name="atps")[:]
                for kk in range(KT):
                    nc.tensor.transpose(atps[:, kk * P : (kk + 1) * P],
                                        attn[:, 2 * kk : 2 * kk + 2, :], ident16)
                a3 = atps.rearrange("p (k l) -> p k l", k=KT)
                nc.scalar.activation(ahl[:, :, 0, lt * P : (lt + 1) * P], a3,
                                     AF.Copy, scale=SA)
                nc.vector.scalar_tensor_tensor(
                    ahl[:, :, 1, lt * P : (lt + 1) * P], a3, SA,
                    ahl[:, :, 0, lt * P : (lt + 1) * P], AL.mult, AL.subtract)
                pend_y.append((ich, ahl, lt))
        for args in pend_y:
            emit_y(*args)


_CACHE = {}


def _get_nc(L=4096, hqb=False, hpb=False):
    key = ("nc", L, hqb, hpb)
    if key not in _CACHE:
        _CACHE[key] = build(L, hqb, hpb)
    return _CACHE[key]


last_exec_time_ns = None
last_profile = None


def kernel(x, qkv_w, qkv_b, proj_w, proj_b, proj_mat):
    global last_exec_time_ns, last_profile
    from concourse.bass_utils import run_bass_kernel_spmd

    x = np.asarray(x, np.float32)
    B, L, _ = x.shape
    hqb = bool(np.any(np.asarray(qkv_b)))
    hpb = bool(np.any(np.asarray(proj_b)))
    nc = _get_nc(L, hqb, hpb)
    base = {
        "qkv_w": np.ascontiguousarray(np.asarray(qkv_w, np.float32)),
        "qkv_b": np.ascontiguousarray(np.asarray(qkv_b, np.float32)),
        "proj_w": np.ascontiguousarray(np.asarray(proj_w, np.float32)),
        "proj_b": np.ascontiguousarray(np.asarray(proj_b, np.float32)),
        "proj_mat": np.ascontiguousarray(np.asarray(proj_mat, np.float32)),
    }
    in_maps = [dict(base, x=np.ascontiguousarray(x[b])) for b in range(B)]
    trace = bool(int(os.environ.get("KERNEL_TRACE", "0")))
    res = run_bass_kernel_spmd(nc, in_maps, core_ids=list(range(B)), trace=trace)
    last_exec_time_ns = res.exec_time_ns
    last_profile = res.profile_json
    return np.stack([res.results[b]["y"] for b in range(B)], axis=0)


def _ref_np(x, qkv_w, qkv_b, proj_w, proj_b, proj_mat):
    Ls = x.shape[0]
    qkv = x @ qkv_w.T + qkv_b
    qkv = qkv.reshape(Ls, 3, H, D)
    q, k, v = qkv[:, 0], qkv[:, 1], qkv[:, 2]
    qp = np.maximum(RATIO * np.einsum("lhd,md->lhm", q, proj_mat), 0) + EPS
    kp = np.maximum(RATIO * np.einsum("lhd,md->lhm", k, proj_mat), 0) + EPS
    kv = np.einsum("lhm,lhd->hmd", kp, v)
    ks = kp.sum(axis=0)
    num = np.einsum("lhm,hmd->lhd", qp, kv)
    den = np.einsum("lhm,hm->lh", qp, ks)
    out = (num / den[..., None]).reshape(Ls, DIM)
    return out @ proj_w.T + proj_b


if __name__ == "__main__":
    from concourse.bass_interp import CoreSim

    Ls = int(os.environ.get("SIM_L", "512"))
    use_bias = bool(int(os.environ.get("SIM_BIAS", "1")))
    rng = np.random.default_rng(0)
    x = rng.standard_normal((Ls, DIM), dtype=np.float32)
    qkv_w = (rng.standard_normal((3 * DIM, DIM), dtype=np.float32) * DIM**-0.5)
    qkv_b = (rng.standard_normal(3 * DIM, dtype=np.float32) * 0.1
             if use_bias else np.zeros(3 * DIM, np.float32))
    proj_w = (rng.standard_normal((DIM, DIM), dtype=np.float32) * DIM**-0.5)
    proj_b = (rng.standard_normal(DIM, dtype=np.float32) * 0.1
              if use_bias else np.zeros(DIM, np.float32))
    proj_mat = rng.standard_normal((M, D), dtype=np.float32)

    print(f"building L={Ls} bias={use_bias} ...")
    nc = build(Ls, use_bias, use_bias)
    print("simulating ...")
    sim = CoreSim(nc)
    for name, arr in [("x", x), ("qkv_w", qkv_w), ("qkv_b", qkv_b),
                      ("proj_w", proj_w), ("proj_b", proj_b),
                      ("proj_mat", proj_mat)]:
        sim.tensor(name)[:] = arr
    sim.simulate(check_with_hw=False)
    got = np.array(sim.tensor("y"))
    want = _ref_np(x, qkv_w, qkv_b, proj_w, proj_b, proj_mat)
    err = np.abs(got - want)
    rel = np.linalg.norm(got - want) / np.linalg.norm(want)
    print("max abs err:", err.max(), " rel fro err:", rel)
    assert rel < 2e-2, "sim mismatch"
    print("SIM OK")


# revision 20
# speedup vs baseline: 1.0416x; 1.0124x over previous
"""FAVOR+ (Performer) non-causal linear attention on 8 Trainium2 NeuronCores.

Sharding: data-parallel over batch B=8 -> one batch element per core.

Per-core pipeline (L=4096, DIM=768, H=12, D=64, M=256):
  prep : cast-DMA weights to fp16, PE-transpose to feature-major, split into
         fp8e4m3 hi/lo pairs (scaled) for DoubleRow matmuls; DMA blocks
         interleaved with chunk-0/1 transposes and v so PE never starves
  pass1: per 512-row chunk: cast-DMA x to fp16; PE-transpose -> xT hi/lo fp8;
         kT/qT/v via fp8 DoubleRow hi/lo-compensated GEMMs (4.5 cyc per
         128x512 output tile instead of 6); k features fp16 with blockdiag pm
         (2 heads per matmul); kv accumulated m-major [m,65] into 4 persistent
         PSUM banks across all chunks (ones-augmented v gives k_sum for
         free); qT staged in SBUF fp16 (no DRAM round trip).  Emission is
         software-pipelined: transposes/v run 2 chunks ahead; kv trails one
         pair behind its kp conversion.
  mid  : kv PSUM -> fp16 SBUF (already m-major); eps*colsum(kv) rows for the
         ACT-assigned q-feature heads
  pass2: q features m-major fp16 (computed one chunk ahead, spread across the
         l-tile loop; relu+eps on DVE/Pool, plain relu on ACT with the eps
         restored by a rank-1 matmul into num); num L-major [l,65] (den =
         col 64); reciprocal + broadcast multiply on DVE; attn PE-transposed
         to feature-major, split fp8 hi/lo; y via DoubleRow GEMM -> DMA out
"""

import math
import os
import sys
from contextlib import ExitStack

import numpy as np

for _p in ("/opt/trn_rl_repo",):
    if _p not in sys.path and os.path.isdir(_p):
        sys.path.insert(0, _p)

import concourse.bass as bass  # noqa: E402
import concourse.mybir as mybir  # noqa: E402
import concourse.tile as tile  # noqa: E402
from concourse import bacc  # noqa: E402

P = 128
DIM = 768
H = 12
D = 64
M = 256
KT = DIM // P   # 6 contraction c-tiles
NPAIR = H // 2  # 6 head pairs
EPS = 1e-3
RATIO = 1.0 / math.sqrt(float(M))

SX = 16.0    # x ~ N(0,1)
SW = 32.0    # qkv_w ~ N(0, 1/768)
SA = 64.0    # attn ~ 0.1
SP = 32.0    # proj_w ~ N(0, 1/768)
SKT = 16.0   # kT ~ N(0,1) -> fp8 for the k-feature GEMM
SPM = 32.0   # RATIO*pm ~ N(0, 1/256) -> fp8
SKP = SKT * SPM  # k-feature path runs in this scaled domain until kvm

F32 = mybir.dt.float32
F16 = mybir.dt.float16
F8 = mybir.dt.float8e4
AL = mybir.AluOpType
AF = mybir.ActivationFunctionType
DR = mybir.MatmulPerfMode.DoubleRow

# pass-2 q-feature conversion engine per (head, mtile) slot ai=h*2+mt.
# Slots of the group-starting heads (ai 0,1,12,13) stay off ACT so each num
# PSUM group's first instruction is its start matmul.
_QP_ENG = {}
_c = 0
for _ai in range(2 * H):
    if _ai in (0, 1, 12, 13):
        _QP_ENG[_ai] = "dve"
    else:
        _QP_ENG[_ai] = ("dve", "act", "act")[_c % 3]
        _c += 1


def build(L=4096, has_qkv_b=False, has_proj_b=False):
    LCH = 512
    NCH = L // LCH
    NSUB = LCH // P  # 4

    nc = bacc.Bacc("TRN2", target_bir_lowering=False, debug=False)
    x_d = nc.dram_tensor("x", [L, DIM], F32, kind="ExternalInput").ap()
    qkvw_d = nc.dram_tensor("qkv_w", [3 * DIM, DIM], F32, kind="ExternalInput").ap()
    qkvb_d = nc.dram_tensor("qkv_b", [3 * DIM], F32, kind="ExternalInput").ap()
    projw_d = nc.dram_tensor("proj_w", [DIM, DIM], F32, kind="ExternalInput").ap()
    projb_d = nc.dram_tensor("proj_b", [DIM], F32, kind="ExternalInput").ap()
    pm_d = nc.dram_tensor("proj_mat", [M, D], F32, kind="ExternalInput").ap()
    y_d = nc.dram_tensor("y", [L, DIM], F32, kind="ExternalOutput").ap()

    with tile.TileContext(nc) as tc:
        with ExitStack() as ctx:
            _body(ctx, tc, x_d, qkvw_d, qkvb_d, projw_d, projb_d, pm_d, y_d,
                  L, LCH, NCH, NSUB, has_qkv_b, has_proj_b)
    nc.compile()
    return nc


def _dr_gemm(nc, out, whl, xhl, wcols, lt=None, bias=None):
    """Accumulating fp8 DoubleRow hi/lo-compensated GEMM over 768 contraction.

    whl/xhl: [128, KT, 2, *] fp8 with hi at [:,kk,0,:], lo at [:,kk,1,:].
    lt=None : out[wc, l]; stationary = whl cols wcols, moving = xhl  (kT/qT)
    lt given: out[l, wc]; stationary = xhl l-tile cols, moving = whl (v/y)
    """
    c0, c1 = wcols
    n = c1 - c0
    if lt is None:
        for i in range(KT // 2):
            for kk in (2 * i, 2 * i + 1):
                nc.tensor.matmul(
                    out, whl[:, kk, :, c0:c1],
                    xhl[:, kk, 0:1, :].to_broadcast([P, 2, out.shape[-1]]),
                    start=(kk == 0), stop=False, perf_mode=DR,
                )
            nc.tensor.matmul(
                out, whl[:, 2 * i : 2 * i + 2, 0, c0:c1],
                xhl[:, 2 * i : 2 * i + 2, 1, :],
                start=False, stop=(bias is None and i == KT // 2 - 1),
                perf_mode=DR,
            )
    else:
        l0 = lt * P
        for i in range(KT // 2):
            for kk in (2 * i, 2 * i + 1):
                nc.tensor.matmul(
                    out, xhl[:, kk, :, l0 : l0 + P],
                    whl[:, kk, 0:1, c0:c1].to_broadcast([P, 2, n]),
                    start=(kk == 0), stop=False, perf_mode=DR,
                )
            nc.tensor.matmul(
                out, xhl[:, 2 * i : 2 * i + 2, 0, l0 : l0 + P],
                whl[:, 2 * i : 2 * i + 2, 1, c0:c1],
                start=False, stop=(bias is None and i == KT // 2 - 1),
                perf_mode=DR,
            )
    if bias is not None:
        ones_row, brow = bias
        nc.tensor.matmul(out, ones_row, brow, start=False, stop=True)


def _body(ctx, tc, x_d, qkvw_d, qkvb_d, projw_d, projb_d, pm_d, y_d,
          L, LCH, NCH, NSUB, has_qkv_b, has_proj_b):
    nc = tc.nc
    iqkv = 1.0 / (SX * SW)
    iy = 1.0 / (SA * SP)

    persist = ctx.enter_context(tc.tile_pool(name="persist", bufs=1))

    ident16 = persist.tile([P, P], F16, tag="ident16", name="ident16")[:]
    nc.gpsimd.memset(ident16, 0.0)
    nc.gpsimd.affine_select(
        out=ident16, in_=ident16, compare_op=AL.not_equal, fill=1.0,
        base=0, pattern=[[-1, P]], channel_multiplier=1,
    )
    ones1 = persist.tile([1, P], F16, tag="ones1", name="ones1")[:]
    nc.gpsimd.memset(ones1, 1.0)
    epscol = persist.tile([P, 1], F16, tag="epscol", name="epscol")[:]
    nc.gpsimd.memset(epscol, EPS)
    epsb = persist.tile([P, 1], F32, tag="epsb", name="epsb")[:]
    nc.gpsimd.memset(epsb, SKP * EPS)
    epsq = persist.tile([P, 1], F32, tag="epsq", name="epsq")[:]
    nc.gpsimd.memset(epsq, EPS)

    whl_qk = persist.tile([P, KT, 2, 2 * DIM], F8, tag="whl_qk", name="whl_qk")[:]
    wvhl = persist.tile([P, KT, 2, DIM], F8, tag="wvhl", name="wvhl")[:]
    wphl = persist.tile([P, KT, 2, DIM], F8, tag="wphl", name="wphl")[:]
    # folded blockdiag pm for the fp8 DoubleRow k-feature GEMM:
    # slice 0 = [RATIO*pmT | 0] (c rows 0:64), slice 1 = [0 | RATIO*pmT]
    pmbd = persist.tile([P, 2, 2 * M], F8, tag="pmbd", name="pmbd")[:]
    pmt2 = persist.tile([P, M], F16, tag="pmt2", name="pmt2")[:]
    qt_sb = persist.tile([P, NPAIR, L], F16, tag="qt_sb", name="qt_sb")[:]
    kvm = persist.tile([P, 4, 6, D + 1], F16, tag="kvm", name="kvm")[:]
    kvmcs = persist.tile([1, 4, 6, D + 1], F16, tag="kvmcs", name="kvmcs")[:]

    if has_qkv_b:
        qkb = persist.tile([P, 2 * KT], F32, tag="qkb", name="qkb")[:]
        nc.sync.dma_start(qkb, qkvb_d.rearrange("(t p) -> p t", p=P)[:, 0 : 2 * KT])
        qkbk = persist.tile([P, KT], F32, tag="qkbk", name="qkbk")[:]
        nc.scalar.activation(qkbk, qkb[:, KT : 2 * KT], AF.Copy, scale=SKT)
        vbf = persist.tile([1, DIM], F32, tag="vbf", name="vbf")[:]
        nc.sync.dma_start(vbf, qkvb_d[2 * DIM : 3 * DIM].unsqueeze(0))
        vb_row = persist.tile([1, DIM], F16, tag="vb_row", name="vb_row")[:]
        nc.scalar.activation(vb_row, vbf, AF.Copy, scale=SX * SW)
    if has_proj_b:
        pbf = persist.tile([1, DIM], F32, tag="pbf", name="pbf")[:]
        nc.sync.dma_start(pbf, projb_d.unsqueeze(0))
        pb_row = persist.tile([1, DIM], F16, tag="pb_row", name="pb_row")[:]
        nc.scalar.activation(pb_row, pbf, AF.Copy, scale=SA * SP)

    vsb = persist.tile([P, 2, NSUB, H, D + 1], F16, tag="vsb", name="vsb")[:]
    nc.gpsimd.memset(vsb[:, :, :, :, D : D + 1], 1.0)

    # kv accumulator in SBUF fp32 (indexed by ai = h*2+mt)
    kv_acc = persist.tile([P, 2 * H, D + 1], F32, tag="kv_acc", name="kv_acc")[:]

    # pass-2 q-feature tiles, double-buffered by chunk parity
    qp2 = [persist.tile([P, H, 2, LCH], F16, tag=f"qp2_{i}", name=f"qp2_{i}")[:]
           for i in range(2)]

    def emit_qp(ich, heads, psum_pool, tag):
        l0 = ich * LCH
        qp_sb = qp2[ich % 2]
        for h in heads:
            p, h2 = h // 2, h % 2
            for mt in range(2):
                qps = psum_pool.tile([P, LCH], F32, tag=tag, name="qps")[:]
                nc.tensor.matmul(
                    qps,
                    pmt2[h2 * D : (h2 + 1) * D, mt * P : (mt + 1) * P],
                    qt_sb[h2 * D : (h2 + 1) * D, p, l0 : l0 + LCH],
                    start=True, stop=True)
                eng = _QP_ENG[h * 2 + mt]
                if eng == "act":
                    # relu(z+eps) ~ relu(z)+eps (differs by <= eps only for
                    # z in (-eps, 0))
                    nc.scalar.activation(qp_sb[:, h, mt, :], qps, AF.Relu,
                                         bias=epsq)
                else:
                    nc.vector.tensor_scalar(qp_sb[:, h, mt, :], qps,
                                            EPS, EPS, AL.add, AL.max)

    if True:
        with tc.tile_pool(name="p1x", bufs=2) as xp, \
             tc.tile_pool(name="p1xhl", bufs=2) as xhlp, \
             tc.tile_pool(name="p1kt", bufs=3) as ktp, \
             tc.tile_pool(name="p1kp", bufs=8) as kpp, \
             tc.tile_pool(name="wprep", bufs=3) as wpool, \
             tc.tile_pool(name="p1work", bufs=8, space="PSUM") as work:

            xnats = {}
            xhls = {}

            def dma_x(ich):
                l0 = ich * LCH
                xnat = xp.tile([P, NSUB, DIM], F16, tag="xnat", name="xnat")[:]
                nc.gpsimd.dma_start(
                    xnat,
                    x_d[l0 : l0 + LCH, :].rearrange("(s p) k -> p s k", p=P))
                xnats[ich] = xnat

            xhl_cur = {}

            def transp_x(ich, kks=range(KT)):
                if ich in xhl_cur:
                    xnat, xhl = xhl_cur[ich]
                else:
                    xnat = xnats.pop(ich)
                    xhl = xhlp.tile([P, KT, 2, LCH], F8, tag="xhl",
                                    name="xhl")[:]
                    xhl_cur[ich] = (xnat, xhl)
                for kk in kks:
                    tp = work.tile([P, 512], F16, tag="work", name="ttp")[:]
                    for s in range(NSUB):
                        nc.tensor.transpose(
                            tp[:, s * P : (s + 1) * P],
                            xnat[:, s, kk * P : (kk + 1) * P], ident16)
                    nc.scalar.activation(xhl[:, kk, 0, :], tp, AF.Copy,
                                         scale=SX)
                    nc.vector.scalar_tensor_tensor(
                        xhl[:, kk, 1, :], tp, SX, xhl[:, kk, 0, :],
                        AL.mult, AL.subtract)
                xhls[ich] = xhl

            def emit_v(ich, subs):
                vs = ich % 2
                xhl = xhls[ich]
                for s in subs:
                    for ci in range(2):
                        c0 = ci * 384
                        vps = work.tile([P, 512], F32, tag="work", name="vps")[:]
                        bias = None
                        if has_qkv_b:
                            bias = (ones1, vb_row[:, c0 : c0 + 384])
                        _dr_gemm(nc, vps[:, 0:384], wvhl, xhl, (c0, c0 + 384),
                                 lt=s, bias=bias)
                        nc.scalar.activation(
                            vsb[:, vs, s, 6 * ci : 6 * ci + 6, 0:D],
                            vps[:, 0:384].rearrange("p (h d) -> p h d", d=D),
                            AF.Copy, scale=iqkv)

            def emit_kT(ich, p):
                ktps = work.tile([P, 512], F32, tag="work", name="ktps")[:]
                _dr_gemm(nc, ktps, whl_qk, xhls[ich],
                         (DIM + p * P, DIM + (p + 1) * P))
                # fold [128,512] -> [64, 2, 512] fp8 (scaled) for DoubleRow
                kt = ktp.tile([P, 2, LCH], F8, tag="kt", name="kt")[:]
                for h2 in range(2):
                    if has_qkv_b:
                        nc.scalar.activation(
                            kt[0:D, h2, :], ktps[h2 * D : (h2 + 1) * D, :],
                            AF.Identity,
                            bias=qkbk[h2 * D : (h2 + 1) * D, p : p + 1],
                            scale=SKT * iqkv)
                    else:
                        nc.scalar.activation(
                            kt[0:D, h2, :], ktps[h2 * D : (h2 + 1) * D, :],
                            AF.Copy, scale=SKT * iqkv)
                return kt

            def emit_kp(p, kt):
                kps = []
                for lt in range(NSUB):
                    kpps = work.tile([P, 512], F32, tag="work", name="kpps")[:]
                    nc.tensor.matmul(kpps,
                                     kt[0:D, :, lt * P : (lt + 1) * P],
                                     pmbd[0:D], start=True, stop=True,
                                     perf_mode=DR)
                    kp = kpp.tile([P, 2 * M], F16, tag="kp", name="kp")[:]
                    # k-feature path is scaled by SKP; it cancels in num/den.
                    # ACT-assigned tiles use relu(z+eps) ~ relu(z)+eps
                    # (differs by <= eps only for z in (-eps, 0)); exact
                    # max(z+eps, eps) on DVE for the rest
                    if lt == (2 if p % 2 == 0 else 1):
                        nc.scalar.activation(kp, kpps, AF.Relu, bias=epsb)
                    else:
                        nc.vector.tensor_scalar(kp, kpps, SKP * EPS, SKP * EPS,
                                                AL.add, AL.max)
                    kps.append(kp)
                return kps

            def emit_qT(ich, p):
                l0 = ich * LCH
                qtps = work.tile([P, 512], F32, tag="work", name="qtps")[:]
                _dr_gemm(nc, qtps, whl_qk, xhls[ich], (p * P, (p + 1) * P))
                if has_qkv_b:
                    nc.scalar.activation(qt_sb[:, p, l0 : l0 + LCH], qtps,
                                         AF.Identity,
                                         bias=qkb[:, p : p + 1], scale=iqkv)
                else:
                    nc.scalar.activation(qt_sb[:, p, l0 : l0 + LCH], qtps,
                                         AF.Copy, scale=iqkv)

            def emit_kv(ich, p, kps):
                vs = ich % 2
                kvp = work.tile([P, 4, D + 1], F32, tag="work", name="kvp")[:]
                for lt in range(NSUB):
                    kp = kps[lt]
                    for h2 in range(2):
                        h = 2 * p + h2
                        for mt in range(2):
                            j = h2 * 2 + mt
                            nc.tensor.matmul(
                                kvp[:, j, :],
                                kp[:, j * P : (j + 1) * P],
                                vsb[:, vs, lt, h, :],
                                start=(lt == 0 and j == 0),
                                stop=(lt == NSUB - 1 and j == 3),
                            )
                nc.vector.tensor_tensor(
                    kv_acc[:, 4 * p : 4 * p + 4, :], kvp,
                    kv_acc[:, 4 * p : 4 * p + 4, :], AL.add)

            # ---- prep: weight DMA blocks interleaved with chunk-0/1 work ----
            def prep_w_blocks(src, nrows, dst, dst_off, scale):
                blocks = []
                nt = nrows // P
                c0 = 0
                while c0 < nt:
                    bs = min(3, nt - c0)
                    st = {}

                    def bdma(c0=c0, bs=bs, st=st):
                        wnat = wpool.tile([P, 3, DIM], F16, tag="wnat",
                                          name="wnat")[:]
                        nc.gpsimd.dma_start(
                            wnat[:, 0:bs, :],
                            src[c0 * P : (c0 + bs) * P, :].rearrange(
                                "(s p) k -> p s k", p=P))
                        st["wnat"] = wnat

                    def bcomp(c0=c0, bs=bs, st=st):
                        wnat = st["wnat"]
                        for kk in range(KT):
                            tp = work.tile([P, 512], F16, tag="work",
                                           name="ptp")[:]
                            for j in range(bs):
                                nc.tensor.transpose(
                                    tp[:, j * P : (j + 1) * P],
                                    wnat[:, j, kk * P : (kk + 1) * P], ident16)
                            hi = dst[:, kk, 0,
                                     dst_off + c0 * P : dst_off + (c0 + bs) * P]
                            nc.scalar.activation(hi, tp[:, 0 : bs * P], AF.Copy,
                                                 scale=scale)
                            nc.vector.scalar_tensor_tensor(
                                dst[:, kk, 1,
                                    dst_off + c0 * P : dst_off + (c0 + bs) * P],
                                tp[:, 0 : bs * P], scale, hi,
                                AL.mult, AL.subtract)

                    blocks.append((bdma, bcomp))
                    c0 += bs
                return blocks

            pm_st = {}

            def prep_pm_dma():
                pmn = wpool.tile([P, 2, D], F16, tag="pmn", name="pmn")[:]
                nc.gpsimd.dma_start(pmn, pm_d.rearrange("(s p) d -> p s d", p=P))
                pm_st["pmn"] = pmn

            def prep_pm():
                pmn = pm_st["pmn"]
                tp = work.tile([P, 512], F16, tag="work", name="ptp")[:]
                for s in range(2):
                    nc.tensor.transpose(tp[0:D, s * P : (s + 1) * P],
                                        pmn[:, s, :], ident16)
                nc.gpsimd.memset(pmbd, 0.0)
                nc.scalar.activation(pmbd[0:D, 0, 0:M], tp[0:D, 0:M], AF.Copy,
                                     scale=SPM * RATIO)
                nc.scalar.activation(pmbd[0:D, 1, M : 2 * M], tp[0:D, 0:M],
                                     AF.Copy, scale=SPM * RATIO)
                nc.scalar.activation(pmt2[0:D, :], tp[0:D, 0:M], AF.Copy,
                                     scale=RATIO)
                nc.scalar.activation(pmt2[D:P, :], tp[0:D, 0:M], AF.Copy,
                                     scale=RATIO)

            dma_x(0)
            wv = prep_w_blocks(qkvw_d[2 * DIM : 3 * DIM, :], DIM, wvhl, 0, SW)
            wqk_k = prep_w_blocks(qkvw_d[DIM : 2 * DIM, :], DIM, whl_qk,
                                  DIM, SW)
            wqk_q = prep_w_blocks(qkvw_d[0:DIM, :], DIM, whl_qk, 0, SW)
            wp = prep_w_blocks(projw_d, DIM, wphl, 0, SP)

            nc.gpsimd.memset(kv_acc, 0.0)
            blocks = wv + wqk_k + wqk_q + wp
            bst = {"dma": 0, "comp": 0}

            def bdma_next():
                if bst["dma"] < len(blocks):
                    blocks[bst["dma"]][0]()
                    bst["dma"] += 1

            def bcomp_next():
                if bst["comp"] < len(blocks):
                    blocks[bst["comp"]][1]()
                    bst["comp"] += 1
                    bdma_next()

            nop = lambda: None
            # phase A: wv + k-part of wqk; q-part and proj stream into chunk 0
            nA = len(wv) + len(wqk_k)
            fillers = {
                0: [lambda: transp_x(0, range(0, 4)),
                    (lambda: dma_x(1)) if 1 < NCH else nop],
                1: [lambda: transp_x(0, range(4, KT)), prep_pm],
                2: [lambda: emit_v(0, (0,)), lambda: emit_v(0, (1,)),
                    (lambda: transp_x(1, range(0, 4))) if 1 < NCH else nop],
                3: [lambda: emit_v(0, (2,)), lambda: emit_v(0, (3,)),
                    (lambda: transp_x(1, range(4, KT))) if 1 < NCH else nop,
                    (lambda: dma_x(2)) if 2 < NCH else nop],
            }
            bdma_next()
            prep_pm_dma()
            bdma_next()
            # PE p-state warmup: burn the cold DMA-wait ramping the clock
            wu = work.tile([P, 512], F16, tag="work", name="wu")[:]
            for _ in range(12):
                for s in range(4):
                    nc.tensor.transpose(wu[:, s * P : (s + 1) * P], ident16,
                                        ident16)
            for i in range(nA):
                for f in fillers.get(i, []):
                    f()
                bcomp_next()

            # ---- pass 1 main loop ----
            for ich in range(NCH):
                first = ich == 0
                pend_kt = None
                pend = None
                for p in range(NPAIR):
                    kt = emit_kT(ich, p)
                    if first:
                        # stream remaining weight blocks (q-part + proj)
                        bcomp_next()
                        if p >= 3:
                            emit_qT(0, p - 3)
                    else:
                        emit_qT(ich, p)
                    if pend is not None:
                        emit_kv(ich, pend[0], pend[1])
                        if ich == NCH - 1:
                            pp = pend[0]
                            nc.scalar.activation(
                                kvm.rearrange("p b j c -> p (b j) c")[
                                    :, 4 * pp : 4 * pp + 4, :],
                                kv_acc[:, 4 * pp : 4 * pp + 4, :],
                                AF.Copy, scale=1.0 / SKP)
                        pend = None
                    if pend_kt is not None:
                        pend = (pend_kt[0], emit_kp(pend_kt[0], pend_kt[1]))
                    pend_kt = (p, kt)
                    if ich == NCH - 1 and not first:
                        # chunk-0 q features computed here so pass 2 starts hot
                        emit_qp(0, (2 * p, 2 * p + 1), work, "work")
                    if p == 0 and 1 <= ich and ich + 1 < NCH:
                        transp_x(ich + 1, range(0, 4))
                    if p == 2 and 1 <= ich and ich + 1 < NCH:
                        transp_x(ich + 1, range(4, KT))
                    if p == 3 and 1 <= ich and ich + 2 < NCH:
                        dma_x(ich + 2)
                    if p >= 3 and ich + 1 < NCH:
                        emit_v(ich + 1, (p - 3,))
                if pend is not None:
                    emit_kv(ich, pend[0], pend[1])
                    if ich == NCH - 1:
                        pp = pend[0]
                        nc.scalar.activation(
                            kvm.rearrange("p b j c -> p (b j) c")[
                                :, 4 * pp : 4 * pp + 4, :],
                            kv_acc[:, 4 * pp : 4 * pp + 4, :],
                            AF.Copy, scale=1.0 / SKP)
                pend = (pend_kt[0], emit_kp(pend_kt[0], pend_kt[1]))
                if ich + 1 < NCH:
                    emit_v(ich + 1, (3,))
                emit_kv(ich, pend[0], pend[1])
                if ich == NCH - 1:
                    nc.scalar.activation(
                        kvm.rearrange("p b j c -> p (b j) c")[:, 20:24, :],
                        kv_acc[:, 20:24, :], AF.Copy, scale=1.0 / SKP)
                if first:
                    for p3 in range(3, NPAIR):
                        emit_qT(0, p3)
                    if NCH == 1:
                        emit_qp(0, range(H), work, "work")
                xhls.pop(ich)


    # ---- pass 2 ----
    with tc.tile_pool(name="p2attn", bufs=2) as atp_sb, \
         tc.tile_pool(name="p2rd", bufs=2) as rdp, \
         tc.tile_pool(name="p2ahl", bufs=2) as ahlp, \
         tc.tile_pool(name="p2y", bufs=2) as yp, \
         tc.tile_pool(name="ps2qp", bufs=3, space="PSUM") as qppsum, \
         tc.tile_pool(name="ps2nm", bufs=2, space="PSUM") as numpsum, \
         tc.tile_pool(name="ps2at", bufs=1, space="PSUM") as atpsum, \
         tc.tile_pool(name="ps2y", bufs=1, space="PSUM") as ypsum:

        def emit_y(ich, ahl, lt):
            l0 = ich * LCH
            yps = ypsum.tile([P, DIM], F32, tag="yps", name="yps")[:]
            for c0, c1 in ((0, 512), (512, DIM)):
                b = (ones1, pb_row[:, c0:c1]) if has_proj_b else None
                _dr_gemm(nc, yps[:, c0:c1], wphl, ahl, (c0, c1), lt=lt, bias=b)
            ysb = yp.tile([P, DIM], F32, tag="ysb", name="ysb")[:]
            nc.scalar.activation(ysb, yps, AF.Copy, scale=iy)
            nc.sync.dma_start(y_d[l0 + lt * P : l0 + (lt + 1) * P, :], ysb)

        pend_y = []
        for ich in range(NCH):
            qp_sb = qp2[ich % 2]
            ahl = ahlp.tile([P, KT, 2, LCH], F8, tag="ahl", name="ahl")[:]
            for lt in range(NSUB):
                nmps = [numpsum.tile([P, 6, D + 1], F32, tag="nm", name="nmps")[:]
                        for _ in range(2)]
                for h in range(H):
                    g = h // 6
                    for mt in range(2):
                        ai = h * 2 + mt
                        nc.tensor.matmul(
                            nmps[g][:, h % 6, :],
                            qp_sb[:, h, mt, lt * P : (lt + 1) * P],
                            kvm[:, ai // 6, ai % 6, :],
                            start=(mt == 0 and h % 6 == 0),
                            stop=(mt == 1 and h % 6 == 5))
                rd = rdp.tile([P, H], F32, tag="rd", name="rd")[:]
                attn = atp_sb.tile([P, H, D], F16, tag="attn", name="attn")[:]
                for g in range(2):
                    nc.vector.reciprocal(rd[:, g * 6 : (g + 1) * 6],
                                         nmps[g][:, :, D])
                    nc.vector.tensor_tensor(
                        attn[:, g * 6 : (g + 1) * 6, :],
                        nmps[g][:, :, 0:D],
                        rd[:, g * 6 : (g + 1) * 6, None].to_broadcast([P, 6, D]),
                        AL.mult)
                if ich + 1 < NCH:
                    emit_qp(ich + 1, range(3 * lt, 3 * lt + 3), qppsum, "qps")
                if len(pend_y) >= (2 if ich + 1 < NCH else 1):
                    emit_y(*pend_y.pop(0))
                atps = atpsum.tile([P, DIM], F16, tag="at", name="atps")[:]
                for kk in range(KT):
                    nc.tensor.transpose(atps[:, kk * P : (kk + 1) * P],
                                        attn[:, 2 * kk : 2 * kk + 2, :], ident16)
                a3 = atps.rearrange("p (k l) -> p k l", k=KT)
                nc.scalar.activation(ahl[:, :, 0, lt * P : (lt + 1) * P], a3,
                                     AF.Copy, scale=SA)
                nc.vector.scalar_tensor_tensor(
                    ahl[:, :, 1, lt * P : (lt + 1) * P], a3, SA,
                    ahl[:, :, 0, lt * P : (lt + 1) * P], AL.mult, AL.subtract)
                pend_y.append((ich, ahl, lt))
        for args in pend_y:
            emit_y(*args)


_CACHE = {}


def _get_nc(L=4096, hqb=False, hpb=False):
    key = ("nc", L, hqb, hpb)
    if key not in _CACHE:
        _CACHE[key] = build(L, hqb, hpb)
    return _CACHE[key]


last_exec_time_ns = None
last_profile = None


def kernel(x, qkv_w, qkv_b, proj_w, proj_b, proj_mat):
    global last_exec_time_ns, last_profile
    from concourse.bass_utils import run_bass_kernel_spmd

    x = np.asarray(x, np.float32)
    B, L, _ = x.shape
    hqb = bool(np.any(np.asarray(qkv_b)))
    hpb = bool(np.any(np.asarray(proj_b)))
    nc = _get_nc(L, hqb, hpb)
    base = {
        "qkv_w": np.ascontiguousarray(np.asarray(qkv_w, np.float32)),
        "qkv_b": np.ascontiguousarray(np.asarray(qkv_b, np.float32)),
        "proj_w": np.ascontiguousarray(np.asarray(proj_w, np.float32)),
        "proj_b": np.ascontiguousarray(np.asarray(proj_b, np.float32)),
        "proj_mat": np.ascontiguousarray(np.asarray(proj_mat, np.float32)),
    }
    in_maps = [dict(base, x=np.ascontiguousarray(x[b])) for b in range(B)]
    trace = bool(int(os.environ.get("KERNEL_TRACE", "0")))
    res = run_bass_kernel_spmd(nc, in_maps, core_ids=list(range(B)), trace=trace)
    last_exec_time_ns = res.exec_time_ns
    last_profile = res.profile_json
    return np.stack([res.results[b]["y"] for b in range(B)], axis=0)


def _ref_np(x, qkv_w, qkv_b, proj_w, proj_b, proj_mat):
    Ls = x.shape[0]
    qkv = x @ qkv_w.T + qkv_b
    qkv = qkv.reshape(Ls, 3, H, D)
    q, k, v = qkv[:, 0], qkv[:, 1], qkv[:, 2]
    qp = np.maximum(RATIO * np.einsum("lhd,md->lhm", q, proj_mat), 0) + EPS
    kp = np.maximum(RATIO * np.einsum("lhd,md->lhm", k, proj_mat), 0) + EPS
    kv = np.einsum("lhm,lhd->hmd", kp, v)
    ks = kp.sum(axis=0)
    num = np.einsum("lhm,hmd->lhd", qp, kv)
    den = np.einsum("lhm,hm->lh", qp, ks)
    out = (num / den[..., None]).reshape(Ls, DIM)
    return out @ proj_w.T + proj_b


if __name__ == "__main__":
    from concourse.bass_interp import CoreSim

    Ls = int(os.environ.get("SIM_L", "512"))
    use_bias = bool(int(os.environ.get("SIM_BIAS", "1")))
    rng = np.random.default_rng(0)
    x = rng.standard_normal((Ls, DIM), dtype=np.float32)
    qkv_w = (rng.standard_normal((3 * DIM, DIM), dtype=np.float32) * DIM**-0.5)
    qkv_b = (rng.standard_normal(3 * DIM, dtype=np.float32) * 0.1
             if use_bias else np.zeros(3 * DIM, np.float32))
    proj_w = (rng.standard_normal((DIM, DIM), dtype=np.float32) * DIM**-0.5)
    proj_b = (rng.standard_normal(DIM, dtype=np.float32) * 0.1
              if use_bias else np.zeros(DIM, np.float32))
    proj_mat = rng.standard_normal((M, D), dtype=np.float32)

    print(f"building L={Ls} bias={use_bias} ...")
    nc = build(Ls, use_bias, use_bias)
    print("simulating ...")
    sim = CoreSim(nc)
    for name, arr in [("x", x), ("qkv_w", qkv_w), ("qkv_b", qkv_b),
                      ("proj_w", proj_w), ("proj_b", proj_b),
                      ("proj_mat", proj_mat)]:
        sim.tensor(name)[:] = arr
    sim.simulate(check_with_hw=False)
    got = np.array(sim.tensor("y"))
    want = _ref_np(x, qkv_w, qkv_b, proj_w, proj_b, proj_mat)
    err = np.abs(got - want)
    rel = np.linalg.norm(got - want) / np.linalg.norm(want)
    print("max abs err:", err.max(), " rel fro err:", rel)
    assert rel < 2e-2, "sim mismatch"
    print("SIM OK")


# revision 21
# speedup vs baseline: 1.0517x; 1.0097x over previous
"""FAVOR+ (Performer) non-causal linear attention on 8 Trainium2 NeuronCores.

Sharding: data-parallel over batch B=8 -> one batch element per core.

Per-core pipeline (L=4096, DIM=768, H=12, D=64, M=256):
  prep : cast-DMA weights to fp16, PE-transpose to feature-major, split into
         fp8e4m3 hi/lo pairs (scaled) for DoubleRow matmuls; DMA blocks
         interleaved with chunk-0/1 transposes and v so PE never starves
  pass1: per 512-row chunk: cast-DMA x to fp16; PE-transpose -> xT hi/lo fp8;
         kT/qT/v via fp8 DoubleRow hi/lo-compensated GEMMs (4.5 cyc per
         128x512 output tile instead of 6); k features fp16 with blockdiag pm
         (2 heads per matmul); kv accumulated m-major [m,65] into 4 persistent
         PSUM banks across all chunks (ones-augmented v gives k_sum for
         free); qT staged in SBUF fp16 (no DRAM round trip).  Emission is
         software-pipelined: transposes/v run 2 chunks ahead; kv trails one
         pair behind its kp conversion.
  mid  : kv PSUM -> fp16 SBUF (already m-major); eps*colsum(kv) rows for the
         ACT-assigned q-feature heads
  pass2: q features m-major fp16 (computed one chunk ahead, spread across the
         l-tile loop; relu+eps on DVE/Pool, plain relu on ACT with the eps
         restored by a rank-1 matmul into num); num L-major [l,65] (den =
         col 64); reciprocal + broadcast multiply on DVE; attn PE-transposed
         to feature-major, split fp8 hi/lo; y via DoubleRow GEMM -> DMA out
"""

import math
import os
import sys
from contextlib import ExitStack

import numpy as np

for _p in ("/opt/trn_rl_repo",):
    if _p not in sys.path and os.path.isdir(_p):
        sys.path.insert(0, _p)

import concourse.bass as bass  # noqa: E402
import concourse.mybir as mybir  # noqa: E402
import concourse.tile as tile  # noqa: E402
from concourse import bacc  # noqa: E402

P = 128
DIM = 768
H = 12
D = 64
M = 256
KT = DIM // P   # 6 contraction c-tiles
NPAIR = H // 2  # 6 head pairs
EPS = 1e-3
RATIO = 1.0 / math.sqrt(float(M))

SX = 16.0    # x ~ N(0,1)
SW = 32.0    # qkv_w ~ N(0, 1/768)
SA = 64.0    # attn ~ 0.1
SP = 32.0    # proj_w ~ N(0, 1/768)
SKT = 16.0   # kT ~ N(0,1) -> fp8 for the k-feature GEMM
SPM = 32.0   # RATIO*pm ~ N(0, 1/256) -> fp8
SKP = SKT * SPM  # k-feature path runs in this scaled domain until kvm

F32 = mybir.dt.float32
F16 = mybir.dt.float16
F8 = mybir.dt.float8e4
AL = mybir.AluOpType
AF = mybir.ActivationFunctionType
DR = mybir.MatmulPerfMode.DoubleRow

# pass-2 q-feature conversion engine per (head, mtile) slot ai=h*2+mt.
# Slots of the group-starting heads (ai 0,1,12,13) stay off ACT so each num
# PSUM group's first instruction is its start matmul.
_QP_ENG = {}
_c = 0
for _ai in range(2 * H):
    if _ai in (0, 1, 12, 13):
        _QP_ENG[_ai] = "dve"
    else:
        _QP_ENG[_ai] = ("act", "act", "dve")[_c % 3]
        _c += 1


def build(L=4096, has_qkv_b=False, has_proj_b=False):
    LCH = 512
    NCH = L // LCH
    NSUB = LCH // P  # 4

    nc = bacc.Bacc("TRN2", target_bir_lowering=False, debug=False)
    x_d = nc.dram_tensor("x", [L, DIM], F32, kind="ExternalInput").ap()
    qkvw_d = nc.dram_tensor("qkv_w", [3 * DIM, DIM], F32, kind="ExternalInput").ap()
    qkvb_d = nc.dram_tensor("qkv_b", [3 * DIM], F32, kind="ExternalInput").ap()
    projw_d = nc.dram_tensor("proj_w", [DIM, DIM], F32, kind="ExternalInput").ap()
    projb_d = nc.dram_tensor("proj_b", [DIM], F32, kind="ExternalInput").ap()
    pm_d = nc.dram_tensor("proj_mat", [M, D], F32, kind="ExternalInput").ap()
    y_d = nc.dram_tensor("y", [L, DIM], F32, kind="ExternalOutput").ap()

    with tile.TileContext(nc) as tc:
        with ExitStack() as ctx:
            _body(ctx, tc, x_d, qkvw_d, qkvb_d, projw_d, projb_d, pm_d, y_d,
                  L, LCH, NCH, NSUB, has_qkv_b, has_proj_b)
    nc.compile()
    return nc


def _dr_gemm(nc, out, whl, xhl, wcols, lt=None, bias=None):
    """Accumulating fp8 DoubleRow hi/lo-compensated GEMM over 768 contraction.

    whl/xhl: [128, KT, 2, *] fp8 with hi at [:,kk,0,:], lo at [:,kk,1,:].
    lt=None : out[wc, l]; stationary = whl cols wcols, moving = xhl  (kT/qT)
    lt given: out[l, wc]; stationary = xhl l-tile cols, moving = whl (v/y)
    """
    c0, c1 = wcols
    n = c1 - c0
    if lt is None:
        for i in range(KT // 2):
            for kk in (2 * i, 2 * i + 1):
                nc.tensor.matmul(
                    out, whl[:, kk, :, c0:c1],
                    xhl[:, kk, 0:1, :].to_broadcast([P, 2, out.shape[-1]]),
                    start=(kk == 0), stop=False, perf_mode=DR,
                )
            nc.tensor.matmul(
                out, whl[:, 2 * i : 2 * i + 2, 0, c0:c1],
                xhl[:, 2 * i : 2 * i + 2, 1, :],
                start=False, stop=(bias is None and i == KT // 2 - 1),
                perf_mode=DR,
            )
    else:
        l0 = lt * P
        for i in range(KT // 2):
            for kk in (2 * i, 2 * i + 1):
                nc.tensor.matmul(
                    out, xhl[:, kk, :, l0 : l0 + P],
                    whl[:, kk, 0:1, c0:c1].to_broadcast([P, 2, n]),
                    start=(kk == 0), stop=False, perf_mode=DR,
                )
            nc.tensor.matmul(
                out, xhl[:, 2 * i : 2 * i + 2, 0, l0 : l0 + P],
                whl[:, 2 * i : 2 * i + 2, 1, c0:c1],
                start=False, stop=(bias is None and i == KT // 2 - 1),
                perf_mode=DR,
            )
    if bias is not None:
        ones_row, brow = bias
        nc.tensor.matmul(out, ones_row, brow, start=False, stop=True)


def _body(ctx, tc, x_d, qkvw_d, qkvb_d, projw_d, projb_d, pm_d, y_d,
          L, LCH, NCH, NSUB, has_qkv_b, has_proj_b):
    nc = tc.nc
    iqkv = 1.0 / (SX * SW)
    iy = 1.0 / (SA * SP)

    persist = ctx.enter_context(tc.tile_pool(name="persist", bufs=1))

    ident16 = persist.tile([P, P], F16, tag="ident16", name="ident16")[:]
    nc.gpsimd.memset(ident16, 0.0)
    nc.gpsimd.affine_select(
        out=ident16, in_=ident16, compare_op=AL.not_equal, fill=1.0,
        base=0, pattern=[[-1, P]], channel_multiplier=1,
    )
    ones1 = persist.tile([1, P], F16, tag="ones1", name="ones1")[:]
    nc.gpsimd.memset(ones1, 1.0)
    epscol = persist.tile([P, 1], F16, tag="epscol", name="epscol")[:]
    nc.gpsimd.memset(epscol, EPS)
    epsb = persist.tile([P, 1], F32, tag="epsb", name="epsb")[:]
    nc.gpsimd.memset(epsb, SKP * EPS)
    epsq = persist.tile([P, 1], F32, tag="epsq", name="epsq")[:]
    nc.gpsimd.memset(epsq, EPS)

    whl_qk = persist.tile([P, KT, 2, 2 * DIM], F8, tag="whl_qk", name="whl_qk")[:]
    wvhl = persist.tile([P, KT, 2, DIM], F8, tag="wvhl", name="wvhl")[:]
    wphl = persist.tile([P, KT, 2, DIM], F8, tag="wphl", name="wphl")[:]
    # folded blockdiag pm for the fp8 DoubleRow k-feature GEMM:
    # slice 0 = [RATIO*pmT | 0] (c rows 0:64), slice 1 = [0 | RATIO*pmT]
    pmbd = persist.tile([P, 2, 2 * M], F8, tag="pmbd", name="pmbd")[:]
    pmt2 = persist.tile([P, M], F16, tag="pmt2", name="pmt2")[:]
    qt_sb = persist.tile([P, NPAIR, L], F16, tag="qt_sb", name="qt_sb")[:]
    kvm = persist.tile([P, 4, 6, D + 1], F16, tag="kvm", name="kvm")[:]
    kvmcs = persist.tile([1, 4, 6, D + 1], F16, tag="kvmcs", name="kvmcs")[:]

    if has_qkv_b:
        qkb = persist.tile([P, 2 * KT], F32, tag="qkb", name="qkb")[:]
        nc.sync.dma_start(qkb, qkvb_d.rearrange("(t p) -> p t", p=P)[:, 0 : 2 * KT])
        qkbk = persist.tile([P, KT], F32, tag="qkbk", name="qkbk")[:]
        nc.scalar.activation(qkbk, qkb[:, KT : 2 * KT], AF.Copy, scale=SKT)
        vbf = persist.tile([1, DIM], F32, tag="vbf", name="vbf")[:]
        nc.sync.dma_start(vbf, qkvb_d[2 * DIM : 3 * DIM].unsqueeze(0))
        vb_row = persist.tile([1, DIM], F16, tag="vb_row", name="vb_row")[:]
        nc.scalar.activation(vb_row, vbf, AF.Copy, scale=SX * SW)
    if has_proj_b:
        pbf = persist.tile([1, DIM], F32, tag="pbf", name="pbf")[:]
        nc.sync.dma_start(pbf, projb_d.unsqueeze(0))
        pb_row = persist.tile([1, DIM], F16, tag="pb_row", name="pb_row")[:]
        nc.scalar.activation(pb_row, pbf, AF.Copy, scale=SA * SP)

    vsb = persist.tile([P, 2, NSUB, H, D + 1], F16, tag="vsb", name="vsb")[:]
    nc.gpsimd.memset(vsb[:, :, :, :, D : D + 1], 1.0)

    # kv accumulator in SBUF fp32 (indexed by ai = h*2+mt)
    kv_acc = persist.tile([P, 2 * H, D + 1], F32, tag="kv_acc", name="kv_acc")[:]

    # pass-2 q-feature tiles, double-buffered by chunk parity
    qp2 = [persist.tile([P, H, 2, LCH], F16, tag=f"qp2_{i}", name=f"qp2_{i}")[:]
           for i in range(2)]

    def emit_qp(ich, heads, psum_pool, tag):
        l0 = ich * LCH
        qp_sb = qp2[ich % 2]
        for h in heads:
            p, h2 = h // 2, h % 2
            for mt in range(2):
                qps = psum_pool.tile([P, LCH], F32, tag=tag, name="qps")[:]
                nc.tensor.matmul(
                    qps,
                    pmt2[h2 * D : (h2 + 1) * D, mt * P : (mt + 1) * P],
                    qt_sb[h2 * D : (h2 + 1) * D, p, l0 : l0 + LCH],
                    start=True, stop=True)
                eng = _QP_ENG[h * 2 + mt]
                if eng == "act":
                    # relu(z+eps) ~ relu(z)+eps (differs by <= eps only for
                    # z in (-eps, 0))
                    nc.scalar.activation(qp_sb[:, h, mt, :], qps, AF.Relu,
                                         bias=epsq)
                else:
                    nc.vector.tensor_scalar(qp_sb[:, h, mt, :], qps,
                                            EPS, EPS, AL.add, AL.max)

    if True:
        with tc.tile_pool(name="p1x", bufs=2) as xp, \
             tc.tile_pool(name="p1xhl", bufs=2) as xhlp, \
             tc.tile_pool(name="p1kt", bufs=3) as ktp, \
             tc.tile_pool(name="p1kp", bufs=8) as kpp, \
             tc.tile_pool(name="wprep", bufs=3) as wpool, \
             tc.tile_pool(name="p1work", bufs=8, space="PSUM") as work:

            xnats = {}
            xhls = {}

            def dma_x(ich):
                l0 = ich * LCH
                xnat = xp.tile([P, NSUB, DIM], F16, tag="xnat", name="xnat")[:]
                nc.gpsimd.dma_start(
                    xnat,
                    x_d[l0 : l0 + LCH, :].rearrange("(s p) k -> p s k", p=P))
                xnats[ich] = xnat

            xhl_cur = {}

            def transp_x(ich, kks=range(KT)):
                if ich in xhl_cur:
                    xnat, xhl = xhl_cur[ich]
                else:
                    xnat = xnats.pop(ich)
                    xhl = xhlp.tile([P, KT, 2, LCH], F8, tag="xhl",
                                    name="xhl")[:]
                    xhl_cur[ich] = (xnat, xhl)
                for kk in kks:
                    tp = work.tile([P, 512], F16, tag="work", name="ttp")[:]
                    for s in range(NSUB):
                        nc.tensor.transpose(
                            tp[:, s * P : (s + 1) * P],
                            xnat[:, s, kk * P : (kk + 1) * P], ident16)
                    nc.scalar.activation(xhl[:, kk, 0, :], tp, AF.Copy,
                                         scale=SX)
                    nc.vector.scalar_tensor_tensor(
                        xhl[:, kk, 1, :], tp, SX, xhl[:, kk, 0, :],
                        AL.mult, AL.subtract)
                xhls[ich] = xhl

            def emit_v(ich, subs):
                vs = ich % 2
                xhl = xhls[ich]
                for s in subs:
                    for ci in range(2):
                        c0 = ci * 384
                        vps = work.tile([P, 512], F32, tag="work", name="vps")[:]
                        bias = None
                        if has_qkv_b:
                            bias = (ones1, vb_row[:, c0 : c0 + 384])
                        _dr_gemm(nc, vps[:, 0:384], wvhl, xhl, (c0, c0 + 384),
                                 lt=s, bias=bias)
                        nc.scalar.activation(
                            vsb[:, vs, s, 6 * ci : 6 * ci + 6, 0:D],
                            vps[:, 0:384].rearrange("p (h d) -> p h d", d=D),
                            AF.Copy, scale=iqkv)

            def emit_kT(ich, p):
                ktps = work.tile([P, 512], F32, tag="work", name="ktps")[:]
                _dr_gemm(nc, ktps, whl_qk, xhls[ich],
                         (DIM + p * P, DIM + (p + 1) * P))
                # fold [128,512] -> [64, 2, 512] fp8 (scaled) for DoubleRow
                kt = ktp.tile([P, 2, LCH], F8, tag="kt", name="kt")[:]
                for h2 in range(2):
                    if has_qkv_b:
                        nc.scalar.activation(
                            kt[0:D, h2, :], ktps[h2 * D : (h2 + 1) * D, :],
                            AF.Identity,
                            bias=qkbk[h2 * D : (h2 + 1) * D, p : p + 1],
                            scale=SKT * iqkv)
                    else:
                        nc.scalar.activation(
                            kt[0:D, h2, :], ktps[h2 * D : (h2 + 1) * D, :],
                            AF.Copy, scale=SKT * iqkv)
                return kt

            def emit_kp(p, kt):
                kps = []
                for lt in range(NSUB):
                    kpps = work.tile([P, 512], F32, tag="work", name="kpps")[:]
                    nc.tensor.matmul(kpps,
                                     kt[0:D, :, lt * P : (lt + 1) * P],
                                     pmbd[0:D], start=True, stop=True,
                                     perf_mode=DR)
                    kp = kpp.tile([P, 2 * M], F16, tag="kp", name="kp")[:]
                    # k-feature path is scaled by SKP; it cancels in num/den.
                    # ACT-assigned tiles use relu(z+eps) ~ relu(z)+eps
                    # (differs by <= eps only for z in (-eps, 0)); exact
                    # max(z+eps, eps) on DVE for the rest
                    if lt == (2 if p % 2 == 0 else 1):
                        nc.scalar.activation(kp, kpps, AF.Relu, bias=epsb)
                    else:
                        nc.vector.tensor_scalar(kp, kpps, SKP * EPS, SKP * EPS,
                                                AL.add, AL.max)
                    kps.append(kp)
                return kps

            def emit_qT(ich, p):
                l0 = ich * LCH
                qtps = work.tile([P, 512], F32, tag="work", name="qtps")[:]
                _dr_gemm(nc, qtps, whl_qk, xhls[ich], (p * P, (p + 1) * P))
                if has_qkv_b:
                    nc.scalar.activation(qt_sb[:, p, l0 : l0 + LCH], qtps,
                                         AF.Identity,
                                         bias=qkb[:, p : p + 1], scale=iqkv)
                else:
                    nc.scalar.activation(qt_sb[:, p, l0 : l0 + LCH], qtps,
                                         AF.Copy, scale=iqkv)

            def emit_kv(ich, p, kps):
                vs = ich % 2
                kvp = work.tile([P, 4, D + 1], F32, tag="work", name="kvp")[:]
                for lt in range(NSUB):
                    kp = kps[lt]
                    for h2 in range(2):
                        h = 2 * p + h2
                        for mt in range(2):
                            j = h2 * 2 + mt
                            nc.tensor.matmul(
                                kvp[:, j, :],
                                kp[:, j * P : (j + 1) * P],
                                vsb[:, vs, lt, h, :],
                                start=(lt == 0 and j == 0),
                                stop=(lt == NSUB - 1 and j == 3),
                            )
                nc.vector.tensor_tensor(
                    kv_acc[:, 4 * p : 4 * p + 4, :], kvp,
                    kv_acc[:, 4 * p : 4 * p + 4, :], AL.add)

            # ---- prep: weight DMA blocks interleaved with chunk-0/1 work ----
            def prep_w_blocks(src, nrows, dst, dst_off, scale):
                blocks = []
                nt = nrows // P
                c0 = 0
                while c0 < nt:
                    bs = min(3, nt - c0)
                    st = {}

                    def bdma(c0=c0, bs=bs, st=st):
                        wnat = wpool.tile([P, 3, DIM], F16, tag="wnat",
                                          name="wnat")[:]
                        nc.gpsimd.dma_start(
                            wnat[:, 0:bs, :],
                            src[c0 * P : (c0 + bs) * P, :].rearrange(
                                "(s p) k -> p s k", p=P))
                        st["wnat"] = wnat

                    def bcomp(c0=c0, bs=bs, st=st):
                        wnat = st["wnat"]
                        for kk in range(KT):
                            tp = work.tile([P, 512], F16, tag="work",
                                           name="ptp")[:]
                            for j in range(bs):
                                nc.tensor.transpose(
                                    tp[:, j * P : (j + 1) * P],
                                    wnat[:, j, kk * P : (kk + 1) * P], ident16)
                            hi = dst[:, kk, 0,
                                     dst_off + c0 * P : dst_off + (c0 + bs) * P]
                            nc.scalar.activation(hi, tp[:, 0 : bs * P], AF.Copy,
                                                 scale=scale)
                            nc.vector.scalar_tensor_tensor(
                                dst[:, kk, 1,
                                    dst_off + c0 * P : dst_off + (c0 + bs) * P],
                                tp[:, 0 : bs * P], scale, hi,
                                AL.mult, AL.subtract)

                    blocks.append((bdma, bcomp))
                    c0 += bs
                return blocks

            pm_st = {}

            def prep_pm_dma():
                pmn = wpool.tile([P, 2, D], F16, tag="pmn", name="pmn")[:]
                nc.gpsimd.dma_start(pmn, pm_d.rearrange("(s p) d -> p s d", p=P))
                pm_st["pmn"] = pmn

            def prep_pm():
                pmn = pm_st["pmn"]
                tp = work.tile([P, 512], F16, tag="work", name="ptp")[:]
                for s in range(2):
                    nc.tensor.transpose(tp[0:D, s * P : (s + 1) * P],
                                        pmn[:, s, :], ident16)
                nc.gpsimd.memset(pmbd, 0.0)
                nc.scalar.activation(pmbd[0:D, 0, 0:M], tp[0:D, 0:M], AF.Copy,
                                     scale=SPM * RATIO)
                nc.scalar.activation(pmbd[0:D, 1, M : 2 * M], tp[0:D, 0:M],
                                     AF.Copy, scale=SPM * RATIO)
                nc.scalar.activation(pmt2[0:D, :], tp[0:D, 0:M], AF.Copy,
                                     scale=RATIO)
                nc.scalar.activation(pmt2[D:P, :], tp[0:D, 0:M], AF.Copy,
                                     scale=RATIO)

            dma_x(0)
            wv = prep_w_blocks(qkvw_d[2 * DIM : 3 * DIM, :], DIM, wvhl, 0, SW)
            wqk_k = prep_w_blocks(qkvw_d[DIM : 2 * DIM, :], DIM, whl_qk,
                                  DIM, SW)
            wqk_q = prep_w_blocks(qkvw_d[0:DIM, :], DIM, whl_qk, 0, SW)
            wp = prep_w_blocks(projw_d, DIM, wphl, 0, SP)

            nc.gpsimd.memset(kv_acc, 0.0)
            blocks = wv + wqk_k + wqk_q + wp
            bst = {"dma": 0, "comp": 0}

            def bdma_next():
                if bst["dma"] < len(blocks):
                    blocks[bst["dma"]][0]()
                    bst["dma"] += 1

            def bcomp_next():
                if bst["comp"] < len(blocks):
                    blocks[bst["comp"]][1]()
                    bst["comp"] += 1
                    bdma_next()

            nop = lambda: None
            # phase A: wv + k-part of wqk; q-part and proj stream into chunk 0
            nA = len(wv) + len(wqk_k)
            fillers = {
                0: [lambda: transp_x(0, range(0, 4)),
                    (lambda: dma_x(1)) if 1 < NCH else nop],
                1: [lambda: transp_x(0, range(4, KT)), prep_pm],
                2: [lambda: emit_v(0, (0,)), lambda: emit_v(0, (1,)),
                    (lambda: transp_x(1, range(0, 4))) if 1 < NCH else nop],
                3: [lambda: emit_v(0, (2,)), lambda: emit_v(0, (3,)),
                    (lambda: transp_x(1, range(4, KT))) if 1 < NCH else nop,
                    (lambda: dma_x(2)) if 2 < NCH else nop],
            }
            bdma_next()
            prep_pm_dma()
            bdma_next()
            # PE p-state warmup: burn the cold DMA-wait ramping the clock
            wu = work.tile([P, 512], F16, tag="work", name="wu")[:]
            for _ in range(12):
                for s in range(4):
                    nc.tensor.transpose(wu[:, s * P : (s + 1) * P], ident16,
                                        ident16)
            for i in range(nA):
                for f in fillers.get(i, []):
                    f()
                bcomp_next()

            # ---- pass 1 main loop ----
            for ich in range(NCH):
                first = ich == 0
                pend_kt = None
                pend = None
                for p in range(NPAIR):
                    kt = emit_kT(ich, p)
                    if first:
                        # stream remaining weight blocks (q-part + proj)
                        bcomp_next()
                        if p >= 3:
                            emit_qT(0, p - 3)
                    else:
                        emit_qT(ich, p)
                    if pend is not None:
                        emit_kv(ich, pend[0], pend[1])
                        if ich == NCH - 1:
                            pp = pend[0]
                            nc.scalar.activation(
                                kvm.rearrange("p b j c -> p (b j) c")[
                                    :, 4 * pp : 4 * pp + 4, :],
                                kv_acc[:, 4 * pp : 4 * pp + 4, :],
                                AF.Copy, scale=1.0 / SKP)
                        pend = None
                    if pend_kt is not None:
                        pend = (pend_kt[0], emit_kp(pend_kt[0], pend_kt[1]))
                    pend_kt = (p, kt)
                    if ich == NCH - 1 and not first:
                        # chunk-0 q features computed here so pass 2 starts hot
                        emit_qp(0, (2 * p, 2 * p + 1), work, "work")
                    if p == 0 and 1 <= ich and ich + 1 < NCH:
                        transp_x(ich + 1, range(0, 4))
                    if p == 2 and 1 <= ich and ich + 1 < NCH:
                        transp_x(ich + 1, range(4, KT))
                    if p == 3 and 1 <= ich and ich + 2 < NCH:
                        dma_x(ich + 2)
                    if p >= 3 and ich + 1 < NCH:
                        emit_v(ich + 1, (p - 3,))
                if pend is not None:
                    emit_kv(ich, pend[0], pend[1])
                    if ich == NCH - 1:
                        pp = pend[0]
                        nc.scalar.activation(
                            kvm.rearrange("p b j c -> p (b j) c")[
                                :, 4 * pp : 4 * pp + 4, :],
                            kv_acc[:, 4 * pp : 4 * pp + 4, :],
                            AF.Copy, scale=1.0 / SKP)
                pend = (pend_kt[0], emit_kp(pend_kt[0], pend_kt[1]))
                if ich + 1 < NCH:
                    emit_v(ich + 1, (3,))
                emit_kv(ich, pend[0], pend[1])
                if ich == NCH - 1:
                    nc.scalar.activation(
                        kvm.rearrange("p b j c -> p (b j) c")[:, 20:24, :],
                        kv_acc[:, 20:24, :], AF.Copy, scale=1.0 / SKP)
                if first:
                    for p3 in range(3, NPAIR):
                        emit_qT(0, p3)
                    if NCH == 1:
                        emit_qp(0, range(H), work, "work")
                xhls.pop(ich)


    # ---- pass 2 ----
    with tc.tile_pool(name="p2attn", bufs=2) as atp_sb, \
         tc.tile_pool(name="p2rd", bufs=2) as rdp, \
         tc.tile_pool(name="p2ahl", bufs=2) as ahlp, \
         tc.tile_pool(name="p2y", bufs=2) as yp, \
         tc.tile_pool(name="ps2qp", bufs=3, space="PSUM") as qppsum, \
         tc.tile_pool(name="ps2nm", bufs=2, space="PSUM") as numpsum, \
         tc.tile_pool(name="ps2at", bufs=1, space="PSUM") as atpsum, \
         tc.tile_pool(name="ps2y", bufs=1, space="PSUM") as ypsum:

        def emit_y(ich, ahl, lt):
            l0 = ich * LCH
            yps = ypsum.tile([P, DIM], F32, tag="yps", name="yps")[:]
            for c0, c1 in ((0, 512), (512, DIM)):
                b = (ones1, pb_row[:, c0:c1]) if has_proj_b else None
                _dr_gemm(nc, yps[:, c0:c1], wphl, ahl, (c0, c1), lt=lt, bias=b)
            ysb = yp.tile([P, DIM], F32, tag="ysb", name="ysb")[:]
            nc.scalar.activation(ysb, yps, AF.Copy, scale=iy)
            nc.sync.dma_start(y_d[l0 + lt * P : l0 + (lt + 1) * P, :], ysb)

        pend_y = []
        for ich in range(NCH):
            qp_sb = qp2[ich % 2]
            ahl = ahlp.tile([P, KT, 2, LCH], F8, tag="ahl", name="ahl")[:]
            for lt in range(NSUB):
                nmps = [numpsum.tile([P, 6, D + 1], F32, tag="nm", name="nmps")[:]
                        for _ in range(2)]
                for h in range(H):
                    g = h // 6
                    for mt in range(2):
                        ai = h * 2 + mt
                        nc.tensor.matmul(
                            nmps[g][:, h % 6, :],
                            qp_sb[:, h, mt, lt * P : (lt + 1) * P],
                            kvm[:, ai // 6, ai % 6, :],
                            start=(mt == 0 and h % 6 == 0),
                            stop=(mt == 1 and h % 6 == 5))
                rd = rdp.tile([P, H], F32, tag="rd", name="rd")[:]
                attn = atp_sb.tile([P, H, D], F16, tag="attn", name="attn")[:]
                for g in range(2):
                    nc.vector.reciprocal(rd[:, g * 6 : (g + 1) * 6],
                                         nmps[g][:, :, D])
                    nc.vector.tensor_tensor(
                        attn[:, g * 6 : (g + 1) * 6, :],
                        nmps[g][:, :, 0:D],
                        rd[:, g * 6 : (g + 1) * 6, None].to_broadcast([P, 6, D]),
                        AL.mult)
                if ich + 1 < NCH:
                    emit_qp(ich + 1, range(3 * lt, 3 * lt + 3), qppsum, "qps")
                if len(pend_y) >= (2 if ich + 1 < NCH else 1):
                    emit_y(*pend_y.pop(0))
                atps = atpsum.tile([P, DIM], F16, tag="at", name="atps")[:]
                for kk in range(KT):
                    nc.tensor.transpose(atps[:, kk * P : (kk + 1) * P],
                                        attn[:, 2 * kk : 2 * kk + 2, :], ident16)
                a3 = atps.rearrange("p (k l) -> p k l", k=KT)
                nc.scalar.activation(ahl[:, :, 0, lt * P : (lt + 1) * P], a3,
                                     AF.Copy, scale=SA)
                nc.vector.scalar_tensor_tensor(
                    ahl[:, :, 1, lt * P : (lt + 1) * P], a3, SA,
                    ahl[:, :, 0, lt * P : (lt + 1) * P], AL.mult, AL.subtract)
                pend_y.append((ich, ahl, lt))
        for args in pend_y:
            emit_y(*args)


_CACHE = {}


def _get_nc(L=4096, hqb=False, hpb=False):
    key = ("nc", L, hqb, hpb)
    if key not in _CACHE:
        _CACHE[key] = build(L, hqb, hpb)
    return _CACHE[key]


last_exec_time_ns = None
last_profile = None


def kernel(x, qkv_w, qkv_b, proj_w, proj_b, proj_mat):
    global last_exec_time_ns, last_profile
    from concourse.bass_utils import run_bass_kernel_spmd

    x = np.asarray(x, np.float32)
    B, L, _ = x.shape
    hqb = bool(np.any(np.asarray(qkv_b)))
    hpb = bool(np.any(np.asarray(proj_b)))
    nc = _get_nc(L, hqb, hpb)
    base = {
        "qkv_w": np.ascontiguousarray(np.asarray(qkv_w, np.float32)),
        "qkv_b": np.ascontiguousarray(np.asarray(qkv_b, np.float32)),
        "proj_w": np.ascontiguousarray(np.asarray(proj_w, np.float32)),
        "proj_b": np.ascontiguousarray(np.asarray(proj_b, np.float32)),
        "proj_mat": np.ascontiguousarray(np.asarray(proj_mat, np.float32)),
    }
    in_maps = [dict(base, x=np.ascontiguousarray(x[b])) for b in range(B)]
    trace = bool(int(os.environ.get("KERNEL_TRACE", "0")))
    res = run_bass_kernel_spmd(nc, in_maps, core_ids=list(range(B)), trace=trace)
    last_exec_time_ns = res.exec_time_ns
    last_profile = res.profile_json
    return np.stack([res.results[b]["y"] for b in range(B)], axis=0)


def _ref_np(x, qkv_w, qkv_b, proj_w, proj_b, proj_mat):
    Ls = x.shape[0]
    qkv = x @ qkv_w.T + qkv_b
    qkv = qkv.reshape(Ls, 3, H, D)
    q, k, v = qkv[:, 0], qkv[:, 1], qkv[:, 2]
    qp = np.maximum(RATIO * np.einsum("lhd,md->lhm", q, proj_mat), 0) + EPS
    kp = np.maximum(RATIO * np.einsum("lhd,md->lhm", k, proj_mat), 0) + EPS
    kv = np.einsum("lhm,lhd->hmd", kp, v)
    ks = kp.sum(axis=0)
    num = np.einsum("lhm,hmd->lhd", qp, kv)
    den = np.einsum("lhm,hm->lh", qp, ks)
    out = (num / den[..., None]).reshape(Ls, DIM)
    return out @ proj_w.T + proj_b


if __name__ == "__main__":
    from concourse.bass_interp import CoreSim

    Ls = int(os.environ.get("SIM_L", "512"))
    use_bias = bool(int(os.environ.get("SIM_BIAS", "1")))
    rng = np.random.default_rng(0)
    x = rng.standard_normal((Ls, DIM), dtype=np.float32)
    qkv_w = (rng.standard_normal((3 * DIM, DIM), dtype=np.float32) * DIM**-0.5)
    qkv_b = (rng.standard_normal(3 * DIM, dtype=np.float32) * 0.1
             if use_bias else np.zeros(3 * DIM, np.float32))
    proj_w = (rng.standard_normal((DIM, DIM), dtype=np.float32) * DIM**-0.5)
    proj_b = (rng.standard_normal(DIM, dtype=np.float32) * 0.1
              if use_bias else np.zeros(DIM, np.float32))
    proj_mat = rng.standard_normal((M, D), dtype=np.float32)

    print(f"building L={Ls} bias={use_bias} ...")
    nc = build(Ls, use_bias, use_bias)
    print("simulating ...")
    sim = CoreSim(nc)
    for name, arr in [("x", x), ("qkv_w", qkv_w), ("qkv_b", qkv_b),
                      ("proj_w", proj_w), ("proj_b", proj_b),
                      ("proj_mat", proj_mat)]:
        sim.tensor(name)[:] = arr
    sim.simulate(check_with_hw=False)
    got = np.array(sim.tensor("y"))
    want = _ref_np(x, qkv_w, qkv_b, proj_w, proj_b, proj_mat)
    err = np.abs(got - want)
    rel = np.linalg.norm(got - want) / np.linalg.norm(want)
    print("max abs err:", err.max(), " rel fro err:", rel)
    assert rel < 2e-2, "sim mismatch"
    print("SIM OK")
